# revision 1
# baseline (speedup 1.0000x reference)
"""Trainium2 Bass kernel for nn_Encoder_base (5x ChebConv GNN + pool + MLP).

Distribution over 8 NeuronCores:
  - level-0 ChebConv props: edge-sharded by destination (all 32 batches,
    96 = 32*3 features), selection-matmul scatter + 2 DRAM AllGathers
  - levels 1-3: batch-sharded (4 batches/core, 128 = 4*32 features);
    level-1 props sparse (indirect-DMA row gathers + selection matmuls),
    levels 2-3 dense-S matmuls
  - channel mixes as block-diagonal matmuls in feature-major layout
  - MLP: output-feature sharded (each core owns 512 cols of W6/W7/W8 and
    512 rows of W9), BatchNorm is local per feature; activations AllGathered
"""
import numpy as np
import concourse.bass as bass
import concourse.bacc as bacc
import concourse.tile as tile
from concourse import mybir, bass_utils
from concourse.masks import make_identity

F32 = mybir.dt.float32
I32 = mybir.dt.int32
I16 = mybir.dt.int16
AF = mybir.ActivationFunctionType
ALU = mybir.AluOpType
AX = mybir.AxisListType
RG = [list(range(8))]
NCORES = 8
N0, N1, N2, N3 = 16384, 4096, 1024, 128
EPS = 1e-5

_CACHE = {}


# ---------------------------------------------------------------- host prep
def _prep_prop(row, col, we, n_dest, n_shard):
    """Sorted-by-dest edges -> 128-dest windows, 128-edge chunks, padded so
    chunk counts per window match across shards (one SPMD program)."""
    window = 128
    order = np.argsort(row, kind="stable")
    row, col, we = row[order], col[order], we[order]
    per = n_dest // n_shard
    nwin = per // window
    counts = np.zeros((n_shard, nwin), np.int64)
    lists = {}
    for s in range(n_shard):
        lo = s * per
        for wi in range(nwin):
            wlo = lo + wi * window
            a = np.searchsorted(row, wlo, side="left")
            b = np.searchsorted(row, wlo + window, side="left")
            lists[(s, wi)] = (row[a:b] - wlo, col[a:b], we[a:b])
            counts[s, wi] = (b - a + 127) // 128
    ncw = np.maximum(counts.max(axis=0), 1)
    C = int(ncw.sum())
    src = np.zeros((n_shard, C, 128), np.int32)
    dst = np.full((n_shard, C, 128), 200.0, np.float32)
    wea = np.zeros((n_shard, C, 128), np.float32)
    for s in range(n_shard):
        base = 0
        for wi in range(nwin):
            dl, cl, wl = lists[(s, wi)]
            n = len(dl)
            k = int(ncw[wi])
            src[s, base:base + k].reshape(-1)[:n] = cl
            dst[s, base:base + k].reshape(-1)[:n] = dl
            wea[s, base:base + k].reshape(-1)[:n] = wl
            base += k
    return [int(x) for x in ncw], src, dst, wea


def _edge_we(e, n):
    row, col = np.asarray(e[0], np.int64), np.asarray(e[1], np.int64)
    deg = np.bincount(row, minlength=n).astype(np.float32)
    dis = np.where(deg > 0, 1.0 / np.sqrt(np.maximum(deg, 1.0)), 0.0).astype(np.float32)
    return row, col, -(dis[row] * dis[col]).astype(np.float32)


def _sub_edges(row, col, we, pool_idx):
    order = np.argsort(row, kind="stable")
    row, col, we = row[order], col[order], we[order]
    starts = np.searchsorted(row, pool_idx, side="left")
    ends = np.searchsorted(row, pool_idx, side="right")
    nr, ncl, nw = [], [], []
    for i in range(len(pool_idx)):
        s, e = starts[i], ends[i]
        if e > s:
            nr.append(np.full(e - s, i, np.int64))
            ncl.append(col[s:e])
            nw.append(we[s:e])
    return np.concatenate(nr), np.concatenate(ncl), np.concatenate(nw)


def _dense_s(row, col, we, n):
    s = np.zeros((n, n), np.float32)
    np.add.at(s, (row, col), we)
    return s


def _tile_w(w, pack):
    """[K, M] -> [K//(128*pack) * 128, pack*M]: pack K-blocks side by side."""
    k, m = w.shape
    nb = k // 128
    t = w.reshape(nb // pack, pack, 128, m).transpose(0, 2, 1, 3)
    return np.ascontiguousarray(t.reshape((nb // pack) * 128, pack * m))


def _host_prep(inputs):
    d = {k: np.asarray(v) for k, v in inputs.items()}
    x = d["x"].astype(np.float32)
    l0 = np.asarray(d["l0"], np.int64)
    l1 = np.asarray(d["l1"], np.int64)
    l2 = np.asarray(d["l2"], np.int64)

    X0 = np.ascontiguousarray(x.transpose(1, 0, 2).reshape(N0, 96))
    X0p = np.zeros((N0, 128), np.float32)
    X0p[:, :96] = X0
    X0l0T = np.ascontiguousarray(X0[l0].T)  # [96, 4096]

    r0, c0, w0 = _edge_we(d["e0"], N0)
    ncw_p1, src_p1, dst_p1, we_p1 = _prep_prop(r0, c0, w0, N0, NCORES)
    r0s, c0s, w0s = _sub_edges(r0, c0, w0, l0)
    ncw_p2, src_p2, dst_p2, we_p2 = _prep_prop(r0s, c0s, w0s, N1, NCORES)

    r1, c1, w1 = _edge_we(d["e1"], N1)
    ncw_q1, src_q1, dst_q1, we_q1 = _prep_prop(r1, c1, w1, N1, 1)
    r1s, c1s, w1s = _sub_edges(r1, c1, w1, l1)
    ncw_q2, src_q2, dst_q2, we_q2 = _prep_prop(r1s, c1s, w1s, N2, 1)

    r2, c2, w2 = _edge_we(d["e2"], N2)
    S2 = _dense_s(r2, c2, w2, N2)
    S2T = _tile_w(np.ascontiguousarray(S2.T), 8)       # [128, 8192]
    S2l2T = _tile_w(np.ascontiguousarray(S2[l2].T), 8)  # [128, 1024]
    P_l2 = np.zeros((N2, 128), np.float32)
    P_l2[l2, np.arange(128)] = 1.0
    P_l2 = _tile_w(P_l2, 8)                             # [128, 1024]

    r3, c3, w3 = _edge_we(d["e3"], N3)
    S3T = np.ascontiguousarray(_dense_s(r3, c3, w3, N3).T)

    def wmod(W):
        return W[0] - W[2], W[1], 2.0 * W[2]

    Wm1 = wmod(d["Wc1"].astype(np.float32))
    Wm = [wmod(d[f"Wc{i}"].astype(np.float32)) for i in (2, 3, 4, 5)]
    eye4 = np.eye(4, dtype=np.float32)

    per_core = []
    for k in range(NCORES):
        m = {}
        m["X0"] = X0p
        m["X0l0T"] = X0l0T
        m["iota"] = np.tile(np.arange(128, dtype=np.float32), (128, 1))
        m["epsv"] = np.full((128, 1), EPS, np.float32)
        m["l0_idx"] = np.ascontiguousarray(
            np.tile(l0.astype(np.int16).reshape(-1, 16).T, (8, 1)))
        m["l1_idx"] = np.ascontiguousarray(
            np.tile(l1.astype(np.int16).reshape(-1, 16).T, (8, 1)))
        for pref, (src, dst, wea) in (
            ("p1", (src_p1[k], dst_p1[k], we_p1[k])),
            ("p2", (src_p2[k], dst_p2[k], we_p2[k])),
            ("q1", (src_q1[0], dst_q1[0], we_q1[0])),
            ("q2", (src_q2[0], dst_q2[0], we_q2[0])),
        ):
            flat = src.reshape(-1).astype(np.int16)
            m[pref + "_srcw"] = np.ascontiguousarray(
                np.tile(flat.reshape(-1, 16).T, (8, 1)))
            m[pref + "_dst"] = np.ascontiguousarray(dst.transpose(1, 0))
            m[pref + "_we"] = np.ascontiguousarray(wea.transpose(1, 0))
        m["S2T"] = S2T
        m["S2l2T"] = S2l2T
        m["P_l2"] = P_l2
        m["S3T"] = S3T
        for t in range(3):
            bw = np.zeros((96, 128), np.float32)
            for j in range(4):
                bg = 4 * k + j
                bw[3 * bg:3 * bg + 3, 32 * j:32 * j + 32] = Wm1[t]
            m[f"bigw0_{t}"] = bw
        for lev in range(4):
            for t in range(3):
                m[f"bigw{lev + 1}_{t}"] = np.kron(eye4, Wm[lev][t])
        for lev, nm in ((1, "b1"), (2, "b2"), (3, "b3"), (4, "b4"), (5, "b5")):
            m[f"bias{lev}"] = np.tile(d[nm].astype(np.float32), 4).reshape(128, 1)
        for li in (6, 7, 8):
            W = d[f"W{li}"].astype(np.float32)[:, 512 * k:512 * k + 512]
            m[f"w{li}"] = _tile_w(W, 8)  # [512, 4096]
            m[f"g{li}"] = np.ascontiguousarray(
                d[f"g{li}"].astype(np.float32)[512 * k:512 * k + 512].reshape(4, 128).T)
            m[f"be{li}"] = np.ascontiguousarray(
                d[f"be{li}"].astype(np.float32)[512 * k:512 * k + 512].reshape(4, 128).T)
        m["w9"] = _tile_w(d["W9"].astype(np.float32)[512 * k:512 * k + 512], 4)  # [128, 512]
        per_core.append(m)

    meta = {"p1": ncw_p1, "p2": ncw_p2, "q1": ncw_q1, "q2": ncw_q2}
    return per_core, meta


# ---------------------------------------------------------------- device program
def _build_nc(meta, shapes):
    nc = bacc.Bacc("TRN2", target_bir_lowering=False, debug=False, num_devices=NCORES)
    ein = {}
    for name, arr in shapes.items():
        dt = {np.dtype(np.int32): I32, np.dtype(np.int16): I16}.get(arr.dtype, F32)
        ein[name] = nc.dram_tensor(name, list(arr.shape), dt, kind="ExternalInput")
    out_mu = nc.dram_tensor("mu", [128, 32], F32, kind="ExternalOutput")

    tx1_loc = nc.dram_tensor("tx1_loc", [N0 // 8, 128], F32)
    tx1_all = nc.dram_tensor("tx1_all", [N0, 128], F32)
    p2t_loc = nc.dram_tensor("p2t_loc", [96, 512], F32)
    p2t_all = nc.dram_tensor("p2t_all", [8 * 96, 512], F32)
    z1_dram = nc.dram_tensor("z1_dram", [N1, 128], F32)
    t1l1_dram = nc.dram_tensor("t1l1_dram", [N1, 128], F32)
    x6_loc = nc.dram_tensor("x6_loc", [4096, 4], F32)
    x6_all = nc.dram_tensor("x6_all", [8 * 4096, 4], F32)
    h6_loc = nc.dram_tensor("h6_loc", [512, 32], F32)
    h6_all = nc.dram_tensor("h6_all", [4096, 32], F32)
    h7_loc = nc.dram_tensor("h7_loc", [512, 32], F32)
    h7_all = nc.dram_tensor("h7_all", [4096, 32], F32)
    mu_loc = nc.dram_tensor("mu_loc", [128, 32], F32)
    mu_all = nc.dram_tensor("mu_all", [8 * 128, 32], F32)

    with tile.TileContext(nc) as tc:
        with (
            tc.tile_pool(name="const", bufs=1) as cpool,
            tc.tile_pool(name="big", bufs=1) as bigpool,
            tc.tile_pool(name="work", bufs=3) as wpool,
            tc.tile_pool(name="wload", bufs=2) as wlpool,
            tc.tile_pool(name="psA", bufs=3, space="PSUM") as ppool,
            tc.tile_pool(name="psB", bufs=1, space="PSUM") as apool,
        ):
            ident = cpool.tile([128, 128], F32, tag="ident", name="ident")
            make_identity(nc, ident[:])
            iota_t = cpool.tile([128, 128], F32, tag="iota", name="iota")
            nc.sync.dma_start(out=iota_t[:], in_=ein["iota"][:, :])
            eps_t = cpool.tile([128, 1], F32, tag="epsv", name="epsv")
            nc.sync.dma_start(out=eps_t[:], in_=ein["epsv"][:, :])

            def load_const(name):
                t = cpool.tile(list(shapes[name].shape), F32, tag=name)
                nc.sync.dma_start(out=t[:], in_=ein[name][:, :])
                return t

            def load_chunk_arrs(pref, C):
                s = cpool.tile([128, C * 8], I16, tag=pref + "s", name=pref + "s")
                dd = cpool.tile([128, C], F32, tag=pref + "d", name=pref + "d")
                w = cpool.tile([128, C], F32, tag=pref + "w", name=pref + "w")
                nc.sync.dma_start(out=s[:], in_=ein[pref + "_srcw"][:, :])
                nc.sync.dma_start(out=dd[:], in_=ein[pref + "_dst"][:, :])
                nc.sync.dma_start(out=w[:], in_=ein[pref + "_we"][:, :])
                return s, dd, w

            GRP = 16

            def grp_gather(idx_sb, g0, gc, gather_src):
                zb = wpool.tile([128, GRP * 128], F32, tag="zb", name="zb", bufs=3)
                nc.gpsimd.dma_gather(
                    out_ap=zb[:, :gc * 128].rearrange("p (c e) -> p c e", e=128),
                    in_ap=gather_src[:, :],
                    idxs_ap=idx_sb[:, g0 * 8:(g0 + gc) * 8],
                    num_idxs=gc * 128, num_idxs_reg=gc * 128, elem_size=128,
                    single_packet=False)
                return zb

            def mk_sel(eng, dst_ap, we_ap):
                sel = wpool.tile([128, 128], F32, tag="sel", name="sel")
                eng.tensor_scalar(out=sel[:], in0=iota_t[:], scalar1=dst_ap,
                                  scalar2=we_ap, op0=ALU.is_equal, op1=ALU.mult)
                return sel

            def prop_nodemajor(ncw, pref, gather_src, D, evac):
                C = sum(ncw)
                s, dd, w = load_chunk_arrs(pref, C)
                zbs = {}
                for g0 in range(0, C, GRP):
                    gc = min(GRP, C - g0)
                    zbs[g0] = grp_gather(s, g0, gc, gather_src)
                base = 0
                for wi, nch in enumerate(ncw):
                    ps = ppool.tile([128, 512], F32, tag="ps", name="ps")
                    for c in range(nch):
                        cc = base + c
                        zb = zbs[(cc // GRP) * GRP]
                        lo = (cc % GRP) * 128
                        sel = mk_sel(nc.vector, dd[:, cc:cc + 1], w[:, cc:cc + 1])
                        nc.tensor.matmul(out=ps[:, :D], lhsT=sel[:],
                                         rhs=zb[:, lo:lo + D],
                                         start=(c == 0), stop=(c == nch - 1))
                    evac(wi, ps[:, :D])
                    base += nch

            def transp(src_ap, dst_ap):
                p, f = src_ap.shape
                ps = ppool.tile([128, 512], F32, tag="ps", name="ps")
                nc.tensor.transpose(out=ps[:f, :p], in_=src_ap, identity=ident[:])
                nc.scalar.activation(out=dst_ap, in_=ps[:f, :p], func=AF.Copy)

            def gather_T(idx_t, chunks, gather_src, D, outT):
                chunks = list(chunks)
                zb = grp_gather(idx_t, chunks[0], len(chunks), gather_src)
                for ci in range(len(chunks)):
                    transp(zb[:, ci * 128:ci * 128 + D],
                           outT[:, ci * 128:(ci + 1) * 128])

            def einsum_win(bigw, taps, Din, width, out_ap, func, bias_ap):
                ps = ppool.tile([128, 512], F32, tag="ps", name="ps")
                for t in range(3):
                    nc.tensor.matmul(out=ps[:, :width], lhsT=bigw[t][:Din, :],
                                     rhs=taps[t], start=(t == 0), stop=(t == 2))
                f2 = AF.Identity if func == AF.Copy else func
                nc.scalar.activation(out=out_ap, in_=ps[:, :width], func=f2, bias=bias_ap)

            # ================= LEVEL 0 =================
            with nc.named_scope("l0_prop1"):
                def evac_p1(wi, ps_ap):
                    t = wpool.tile([128, 96], F32, tag="ev", name="ev", bufs=6)
                    nc.scalar.activation(out=t[:], in_=ps_ap, func=AF.Copy)
                    nc.sync.dma_start(out=tx1_loc[wi * 128:(wi + 1) * 128, :96], in_=t[:])
                prop_nodemajor(meta["p1"], "p1", ein["X0"], 96, evac_p1)
            with nc.named_scope("ag1"):
                nc.gpsimd.collective_compute(
                    "AllGather", ALU.bypass, replica_groups=RG,
                    ins=[tx1_loc.ap().opt()], outs=[tx1_all.ap().opt()])

            with nc.named_scope("l0_prop2"):
                C2 = sum(meta["p2"])
                s2c, d2c, w2c = load_chunk_arrs("p2", C2)
                zbs2 = {}
                for g0 in range(0, C2, GRP):
                    gc = min(GRP, C2 - g0)
                    zbs2[g0] = grp_gather(s2c, g0, gc, tx1_all)
                p2t_sb = bigpool.tile([96, 512], F32, tag="p2t_sb", name="p2t_sb")
                base = 0
                for wi, nch in enumerate(meta["p2"]):
                    ps = ppool.tile([128, 512], F32, tag="ps", name="ps")
                    for c in range(nch):
                        cc = base + c
                        zb = zbs2[(cc // GRP) * GRP]
                        lo = (cc % GRP) * 128
                        sel = mk_sel(nc.vector, d2c[:, cc:cc + 1], w2c[:, cc:cc + 1])
                        nc.tensor.matmul(out=ps[:96, :128],
                                         lhsT=zb[:, lo:lo + 96], rhs=sel[:],
                                         start=(c == 0), stop=(c == nch - 1))
                    nc.scalar.activation(out=p2t_sb[:, wi * 128:(wi + 1) * 128],
                                         in_=ps[:96, :128], func=AF.Copy)
                    base += nch
                nc.sync.dma_start(out=p2t_loc[:, :], in_=p2t_sb[:])
            with nc.named_scope("ag2"):
                nc.gpsimd.collective_compute(
                    "AllGather", ALU.bypass, replica_groups=RG,
                    ins=[p2t_loc.ap().opt()], outs=[p2t_all.ap().opt()])

            with nc.named_scope("l0_einsum"):
                l0i = cpool.tile([128, 32 * 8], I16, tag="l0i", name="l0i")
                nc.sync.dma_start(out=l0i[:], in_=ein["l0_idx"][:, :])
                bw0 = [load_const(f"bigw0_{t}") for t in range(3)]
                bias1 = load_const("bias1")
                for w in range(8):
                    g0w = wpool.tile([96, 512], F32, tag="g0w", name="g0w")
                    nc.sync.dma_start(out=g0w[:], in_=ein["X0l0T"][:, 512 * w:512 * (w + 1)])
                    g1w = wpool.tile([96, 512], F32, tag="g1w", name="g1w")
                    gather_T(l0i, range(4 * w, 4 * w + 4), tx1_all, 96, g1w)
                    p2w = wpool.tile([96, 512], F32, tag="p2w", name="p2w")
                    nc.sync.dma_start(out=p2w[:], in_=p2t_all[96 * w:96 * (w + 1), :])
                    z1Tw = wpool.tile([128, 512], F32, tag="z1Tw", name="z1Tw")
                    einsum_win(bw0, [g0w[:], g1w[:], p2w[:]], 96, 512,
                               z1Tw[:], AF.Copy, bias1[:, 0:1])
                    for c in range(4):
                        t = wpool.tile([128, 128], F32, tag="z1nc", name="z1nc")
                        transp(z1Tw[:, c * 128:(c + 1) * 128], t[:])
                        r = w * 512 + c * 128
                        nc.sync.dma_start(out=z1_dram[r:r + 128, :], in_=t[:])

            # ================= LEVEL 1 =================
            with nc.named_scope("l1_prop1"):
                def evac_q1(wi, ps_ap):
                    t = wpool.tile([128, 128], F32, tag="ev", name="ev", bufs=6)
                    nc.scalar.activation(out=t[:], in_=ps_ap, func=AF.Copy)
                    nc.sync.dma_start(out=t1l1_dram[wi * 128:(wi + 1) * 128, :], in_=t[:])
                prop_nodemajor(meta["q1"], "q1", z1_dram, 128, evac_q1)

            p2n_l1 = bigpool.tile([128, 8 * 128], F32, tag="p2n_l1", name="p2n_l1")
            with nc.named_scope("l1_prop2"):
                def evac_q2(wi, ps_ap):
                    nc.scalar.activation(out=p2n_l1[:, wi * 128:(wi + 1) * 128],
                                         in_=ps_ap, func=AF.Copy)
                prop_nodemajor(meta["q2"], "q2", t1l1_dram, 128, evac_q2)

            z2n = bigpool.tile([128, 8 * 128], F32, tag="z2n", name="z2n")
            with nc.named_scope("l1_einsum"):
                l1i = cpool.tile([128, 8 * 8], I16, tag="l1i", name="l1i")
                nc.sync.dma_start(out=l1i[:], in_=ein["l1_idx"][:, :])
                z1l1T = bigpool.tile([128, 1024], F32, tag="z1l1T", name="z1l1T")
                gather_T(l1i, range(8), z1_dram, 128, z1l1T)
                t1l1T = bigpool.tile([128, 1024], F32, tag="t1l1T", name="t1l1T")
                gather_T(l1i, range(8), t1l1_dram, 128, t1l1T)
                p2l1T = bigpool.tile([128, 1024], F32, tag="p2l1T", name="p2l1T")
                for c in range(8):
                    transp(p2n_l1[:, c * 128:(c + 1) * 128], p2l1T[:, c * 128:(c + 1) * 128])
                bw1 = [load_const(f"bigw1_{t}") for t in range(3)]
                bias2 = load_const("bias2")
                z2T = bigpool.tile([128, 1024], F32, tag="z2T", name="z2T")
                for w in range(2):
                    einsum_win(bw1, [z1l1T[:, 512 * w:512 * (w + 1)],
                                     t1l1T[:, 512 * w:512 * (w + 1)],
                                     p2l1T[:, 512 * w:512 * (w + 1)]],
                               128, 512, z2T[:, 512 * w:512 * (w + 1)], AF.Tanh, bias2[:, 0:1])
                for c in range(8):
                    transp(z2T[:, c * 128:(c + 1) * 128], z2n[:, c * 128:(c + 1) * 128])

            # ================= LEVEL 2 (dense) =================
            with nc.named_scope("l2"):
                t1_l2 = bigpool.tile([128, 8 * 128], F32, tag="t1_l2", name="t1_l2")
                for half in range(2):
                    s2t = wlpool.tile([128, 4096], F32, tag="wld", name="wld")
                    nc.sync.dma_start(out=s2t[:], in_=ein["S2T"][:, 4096 * half:4096 * (half + 1)])
                    for dc in range(8):
                        ps = ppool.tile([128, 512], F32, tag="ps", name="ps")
                        for kk in range(4):
                            kc = half * 4 + kk
                            nc.tensor.matmul(
                                out=ps[:, :128],
                                lhsT=s2t[:, kk * 1024 + dc * 128: kk * 1024 + dc * 128 + 128],
                                rhs=z2n[:, kc * 128:(kc + 1) * 128],
                                start=(kk == 0), stop=(kk == 3))
                        if half == 0:
                            nc.scalar.activation(out=t1_l2[:, dc * 128:(dc + 1) * 128],
                                                 in_=ps[:, :128], func=AF.Copy)
                        else:
                            nc.vector.tensor_add(t1_l2[:, dc * 128:(dc + 1) * 128],
                                                 t1_l2[:, dc * 128:(dc + 1) * 128],
                                                 ps[:, :128])
                s2l2 = cpool.tile([128, 1024], F32, tag="s2l2", name="s2l2")
                nc.sync.dma_start(out=s2l2[:], in_=ein["S2l2T"][:, :])
                ps = ppool.tile([128, 512], F32, tag="ps", name="ps")
                for kc in range(8):
                    nc.tensor.matmul(out=ps[:, :128], lhsT=s2l2[:, kc * 128:(kc + 1) * 128],
                                     rhs=t1_l2[:, kc * 128:(kc + 1) * 128],
                                     start=(kc == 0), stop=(kc == 7))
                p2n_l2 = wpool.tile([128, 128], F32, tag="p2n_l2", name="p2n_l2")
                nc.scalar.activation(out=p2n_l2[:], in_=ps[:, :128], func=AF.Copy)
                pl2 = cpool.tile([128, 1024], F32, tag="pl2", name="pl2")
                nc.sync.dma_start(out=pl2[:], in_=ein["P_l2"][:, :])
                z2l2T = wpool.tile([128, 128], F32, tag="z2l2T", name="z2l2T")
                psg = ppool.tile([128, 512], F32, tag="ps", name="ps")
                for kc in range(8):
                    nc.tensor.matmul(out=psg[:, :128], lhsT=z2n[:, kc * 128:(kc + 1) * 128],
                                     rhs=pl2[:, kc * 128:(kc + 1) * 128],
                                     start=(kc == 0), stop=(kc == 7))
                nc.scalar.activation(out=z2l2T[:], in_=psg[:, :128], func=AF.Copy)
                t1l2T = wpool.tile([128, 128], F32, tag="t1l2T", name="t1l2T")
                psg2 = ppool.tile([128, 512], F32, tag="ps", name="ps")
                for kc in range(8):
                    nc.tensor.matmul(out=psg2[:, :128], lhsT=t1_l2[:, kc * 128:(kc + 1) * 128],
                                     rhs=pl2[:, kc * 128:(kc + 1) * 128],
                                     start=(kc == 0), stop=(kc == 7))
                nc.scalar.activation(out=t1l2T[:], in_=psg2[:, :128], func=AF.Copy)
                p2l2T = wpool.tile([128, 128], F32, tag="p2l2T", name="p2l2T")
                transp(p2n_l2[:], p2l2T[:])
                bw2 = [load_const(f"bigw2_{t}") for t in range(3)]
                bias3 = load_const("bias3")
                z3T = wpool.tile([128, 128], F32, tag="z3T", name="z3T")
                einsum_win(bw2, [z2l2T[:], t1l2T[:], p2l2T[:]], 128, 128,
                           z3T[:], AF.Tanh, bias3[:, 0:1])
                z3n = wpool.tile([128, 128], F32, tag="z3n", name="z3n")
                transp(z3T[:], z3n[:])

            # ================= LEVEL 3 =================
            with nc.named_scope("l3"):
                s3t = cpool.tile([128, 128], F32, tag="s3t", name="s3t")
                nc.sync.dma_start(out=s3t[:], in_=ein["S3T"][:, :])
                bias4 = load_const("bias4")
                bias5 = load_const("bias5")

                def conv_l3(zn, zT, bw_pref, bias_t, func, keep):
                    t1T = wpool.tile([128, 128], F32, tag=keep + "t1T", name=keep + "t1T")
                    ps = ppool.tile([128, 512], F32, tag="ps", name="ps")
                    nc.tensor.matmul(out=ps[:, :128], lhsT=zn, rhs=s3t[:], start=True, stop=True)
                    nc.scalar.activation(out=t1T[:], in_=ps[:, :128], func=AF.Copy)
                    t1n_ = wpool.tile([128, 128], F32, tag=keep + "t1n", name=keep + "t1n")
                    transp(t1T[:], t1n_[:])
                    p2T_ = wpool.tile([128, 128], F32, tag=keep + "p2T", name=keep + "p2T")
                    ps2 = ppool.tile([128, 512], F32, tag="ps", name="ps")
                    nc.tensor.matmul(out=ps2[:, :128], lhsT=t1n_[:], rhs=s3t[:], start=True, stop=True)
                    nc.scalar.activation(out=p2T_[:], in_=ps2[:, :128], func=AF.Copy)
                    bw = [load_const(f"{bw_pref}_{t}") for t in range(3)]
                    outT = wpool.tile([128, 128], F32, tag=keep + "oT", name=keep + "oT")
                    einsum_win(bw, [zT, t1T[:], p2T_[:]], 128, 128, outT[:], func, bias_t[:, 0:1])
                    outn = wpool.tile([128, 128], F32, tag=keep + "on", name=keep + "on")
                    transp(outT[:], outn[:])
                    return outn, outT

                z4n, z4T = conv_l3(z3n[:], z3T[:], "bigw3", bias4, AF.Tanh, "c4")
                o5n, o5T = conv_l3(z4n[:], z4T[:], "bigw4", bias5, AF.Copy, "c5")

            # ================= MLP input assembly =================
            with nc.named_scope("mlp_in"):
                for j in range(4):
                    ap_out = x6_loc.ap()[:, j:j + 1].rearrange("(n h) o -> n (h o)", h=32)
                    nc.sync.dma_start(out=ap_out, in_=o5n[:, 32 * j:32 * j + 32])
                nc.gpsimd.collective_compute(
                    "AllGather", ALU.bypass, replica_groups=RG,
                    ins=[x6_loc.ap().opt()], outs=[x6_all.ap().opt()])

            # ================= MLP =================
            def mlp_layer(nm, src_sb, out_sb):
                g_t = load_const("g" + nm[1])
                be_t = load_const("be" + nm[1])
                pss = [apool.tile([128, 32], F32, tag=f"acc{m}", name=f"acc{m}") for m in range(4)]
                for i in range(4):
                    wt = wlpool.tile([128, 4096], F32, tag="wld", name="wld")
                    nc.sync.dma_start(out=wt[:], in_=ein[nm][128 * i:128 * (i + 1), :])
                    for a in range(8):
                        kc = i * 8 + a
                        for mm in range(4):
                            nc.tensor.matmul(
                                out=pss[mm][:],
                                lhsT=wt[:, a * 512 + mm * 128: a * 512 + mm * 128 + 128],
                                rhs=src_sb[:, 32 * kc:32 * kc + 32],
                                start=(kc == 0), stop=(kc == 31))
                for mm in range(4):
                    t = wpool.tile([128, 32], F32, tag="b_t", name="b_t")
                    nc.vector.tensor_copy(t[:], pss[mm][:])
                    s1 = wpool.tile([128, 1], F32, tag="b_s1", name="b_s1")
                    nc.vector.tensor_reduce(out=s1[:], in_=t[:], axis=AX.X, op=ALU.add)
                    mu_ = wpool.tile([128, 1], F32, tag="b_mu", name="b_mu")
                    nc.vector.tensor_scalar_mul(mu_[:], s1[:], 1.0 / 32.0)
                    sq = wpool.tile([128, 32], F32, tag="b_sq", name="b_sq")
                    nc.vector.tensor_mul(sq[:], t[:], t[:])
                    s2_ = wpool.tile([128, 1], F32, tag="b_s2", name="b_s2")
                    nc.vector.tensor_reduce(out=s2_[:], in_=sq[:], axis=AX.X, op=ALU.add)
                    var = wpool.tile([128, 1], F32, tag="b_var", name="b_var")
                    nc.vector.scalar_tensor_tensor(out=var[:], in0=mu_[:], scalar=-1.0,
                                                   in1=mu_[:], op0=ALU.mult, op1=ALU.mult)
                    nc.vector.scalar_tensor_tensor(out=var[:], in0=s2_[:], scalar=1.0 / 32.0,
                                                   in1=var[:], op0=ALU.mult, op1=ALU.add)
                    sd = wpool.tile([128, 1], F32, tag="b_sd", name="b_sd")
                    nc.scalar.activation(out=sd[:], in_=var[:], func=AF.Sqrt, bias=eps_t[:, 0:1])
                    rs = wpool.tile([128, 1], F32, tag="b_rs", name="b_rs")
                    nc.vector.reciprocal(rs[:], sd[:])
                    a_ = wpool.tile([128, 1], F32, tag="b_a", name="b_a")
                    nc.vector.tensor_mul(a_[:], rs[:], g_t[:, mm:mm + 1])
                    sh = wpool.tile([128, 1], F32, tag="b_sh", name="b_sh")
                    nc.vector.scalar_tensor_tensor(out=sh[:], in0=mu_[:], scalar=-1.0,
                                                   in1=a_[:], op0=ALU.mult, op1=ALU.mult)
                    nc.vector.tensor_add(sh[:], sh[:], be_t[:, mm:mm + 1])
                    nc.scalar.activation(out=out_sb[:, 32 * mm:32 * mm + 32], in_=t[:],
                                         func=AF.Relu, scale=a_[:, 0:1], bias=sh[:, 0:1])

            with nc.named_scope("mlp6"):
                x6T = bigpool.tile([128, 1024], F32, tag="x6T", name="x6T")
                for kk in range(8):
                    nc.sync.dma_start(
                        out=x6T[:].rearrange("p (c r) -> p c r", r=32)[:, :, 4 * kk:4 * kk + 4],
                        in_=x6_all[4096 * kk:4096 * (kk + 1), :].rearrange(
                            "(c p) j -> p c j", p=128))
                h6 = bigpool.tile([128, 128], F32, tag="h6sb", name="h6sb")
                mlp_layer("w6", x6T, h6)
                nc.sync.dma_start(out=h6_loc.ap().rearrange("(m p) b -> p m b", p=128),
                                  in_=h6[:].rearrange("p (m b) -> p m b", b=32))
                nc.gpsimd.collective_compute(
                    "AllGather", ALU.bypass, replica_groups=RG,
                    ins=[h6_loc.ap().opt()], outs=[h6_all.ap().opt()])
            with nc.named_scope("mlp7"):
                x7T = bigpool.tile([128, 1024], F32, tag="x7T", name="x7T")
                nc.sync.dma_start(out=x7T[:].rearrange("p (c b) -> p c b", b=32),
                                  in_=h6_all[:, :].rearrange("(c p) b -> p c b", p=128))
                h7 = bigpool.tile([128, 128], F32, tag="h7sb", name="h7sb")
                mlp_layer("w7", x7T, h7)
                nc.sync.dma_start(out=h7_loc.ap().rearrange("(m p) b -> p m b", p=128),
                                  in_=h7[:].rearrange("p (m b) -> p m b", b=32))
                nc.gpsimd.collective_compute(
                    "AllGather", ALU.bypass, replica_groups=RG,
                    ins=[h7_loc.ap().opt()], outs=[h7_all.ap().opt()])
            with nc.named_scope("mlp8"):
                x8T = bigpool.tile([128, 1024], F32, tag="x8T", name="x8T")
                nc.sync.dma_start(out=x8T[:].rearrange("p (c b) -> p c b", b=32),
                                  in_=h7_all[:, :].rearrange("(c p) b -> p c b", p=128))
                h8 = bigpool.tile([128, 128], F32, tag="h8sb", name="h8sb")
                mlp_layer("w8", x8T, h8)

            with nc.named_scope("mlp9"):
                w9t = cpool.tile([128, 512], F32, tag="w9t", name="w9t")
                nc.sync.dma_start(out=w9t[:], in_=ein["w9"][:, :])
                ps9 = apool.tile([128, 32], F32, tag="acc0", name="acc0")
                for kc in range(4):
                    nc.tensor.matmul(out=ps9[:], lhsT=w9t[:, kc * 128:(kc + 1) * 128],
                                     rhs=h8[:, 32 * kc:32 * kc + 32],
                                     start=(kc == 0), stop=(kc == 3))
                mu_sb = wpool.tile([128, 32], F32, tag="mu_sb", name="mu_sb")
                nc.scalar.activation(out=mu_sb[:], in_=ps9[:], func=AF.Copy)
                nc.sync.dma_start(out=mu_loc[:, :], in_=mu_sb[:])
                nc.gpsimd.collective_compute(
                    "AllGather", ALU.bypass, replica_groups=RG,
                    ins=[mu_loc.ap().opt()], outs=[mu_all.ap().opt()])
                tot = wpool.tile([128, 32], F32, tag="f_tot", name="f_tot")
                nc.sync.dma_start(out=tot[:], in_=mu_all[0:128, :])
                for k in range(1, 8):
                    pk = wpool.tile([128, 32], F32, tag="f_pk", name="f_pk")
                    nc.sync.dma_start(out=pk[:], in_=mu_all[k * 128:(k + 1) * 128, :])
                    nc.vector.tensor_add(tot[:], tot[:], pk[:])
                s1 = wpool.tile([128, 1], F32, tag="f_s1", name="f_s1")
                nc.vector.tensor_reduce(out=s1[:], in_=tot[:], axis=AX.X, op=ALU.add)
                mu_ = wpool.tile([128, 1], F32, tag="f_mu", name="f_mu")
                nc.vector.tensor_scalar_mul(mu_[:], s1[:], 1.0 / 32.0)
                sq = wpool.tile([128, 32], F32, tag="f_sq", name="f_sq")
                nc.vector.tensor_mul(sq[:], tot[:], tot[:])
                s2_ = wpool.tile([128, 1], F32, tag="f_s2", name="f_s2")
                nc.vector.tensor_reduce(out=s2_[:], in_=sq[:], axis=AX.X, op=ALU.add)
                var = wpool.tile([128, 1], F32, tag="f_var", name="f_var")
                nc.vector.scalar_tensor_tensor(out=var[:], in0=mu_[:], scalar=-1.0,
                                               in1=mu_[:], op0=ALU.mult, op1=ALU.mult)
                nc.vector.scalar_tensor_tensor(out=var[:], in0=s2_[:], scalar=1.0 / 32.0,
                                               in1=var[:], op0=ALU.mult, op1=ALU.add)
                sdf = wpool.tile([128, 1], F32, tag="f_sd", name="f_sd")
                nc.scalar.activation(out=sdf[:], in_=var[:], func=AF.Sqrt, bias=eps_t[:, 0:1])
                rs = wpool.tile([128, 1], F32, tag="f_rs", name="f_rs")
                nc.vector.reciprocal(rs[:], sdf[:])
                neg = wpool.tile([128, 1], F32, tag="f_neg", name="f_neg")
                nc.vector.scalar_tensor_tensor(out=neg[:], in0=mu_[:], scalar=-1.0,
                                               in1=rs[:], op0=ALU.mult, op1=ALU.mult)
                outt = wpool.tile([128, 32], F32, tag="f_out", name="f_out")
                nc.scalar.activation(out=outt[:], in_=tot[:], func=AF.Identity,
                                     scale=rs[:, 0:1], bias=neg[:, 0:1])
                nc.sync.dma_start(out=out_mu[:, :], in_=outt[:])

    nc.compile()
    return nc


# ---------------------------------------------------------------- entry point
def kernel(**inputs) -> np.ndarray:
    per_core, meta = _host_prep(inputs)
    if "prog" not in _CACHE:
        _CACHE["prog"] = _build_nc(meta, per_core[0])
    nc = _CACHE["prog"]
    res = bass_utils.run_bass_kernel_spmd(nc, per_core, core_ids=list(range(NCORES)))
    return np.ascontiguousarray(res.results[0]["mu"].T)



# revision 6
# speedup vs baseline: 1.3006x; 1.3006x over previous
"""Trainium2 Bass kernel for nn_Encoder_base (5x ChebConv GNN + pool + MLP).

Distribution over 8 NeuronCores:
  - level-0 ChebConv props: edge-sharded by destination (all 32 batches,
    96 = 32*3 features), selection-matmul scatter + 2 DRAM AllGathers
  - levels 1-3: batch-sharded (4 batches/core, 128 = 4*32 features);
    level-1 props sparse (indirect-DMA row gathers + selection matmuls),
    levels 2-3 dense-S matmuls
  - channel mixes as block-diagonal matmuls in feature-major layout
  - MLP: output-feature sharded (each core owns 512 cols of W6/W7/W8 and
    512 rows of W9); activations [128k,32] are the matmul lhsT (stationary)
    with W streaming as rhs; BatchNorm per-feature after a PE transpose;
    activations AllGathered between layers.
All matmul operands are bf16 (fp32 PSUM accumulation); selection matrices
are precomputed on the host and streamed from DRAM.
"""
import numpy as np
import ml_dtypes
import concourse.bass as bass
import concourse.bacc as bacc
import concourse.tile as tile
from concourse import mybir, bass_utils

F32 = mybir.dt.float32
BF16 = mybir.dt.bfloat16
I16 = mybir.dt.int16
AF = mybir.ActivationFunctionType
ALU = mybir.AluOpType
AX = mybir.AxisListType
RG = [list(range(8))]
NCORES = 8
N0, N1, N2, N3 = 16384, 4096, 1024, 128
EPS = 1e-5
BF = ml_dtypes.bfloat16

_CACHE = {}


# ---------------------------------------------------------------- host prep
def _prep_prop(row, col, we, n_dest, n_shard):
    """Sorted-by-dest edges -> 128-dest windows, 128-edge chunks, padded so
    chunk counts per window match across shards (one SPMD program).
    Emits per-chunk selection matrices sel[chunk, edge_local, dst_local]."""
    window = 128
    order = np.argsort(row, kind="stable")
    row, col, we = row[order], col[order], we[order]
    per = n_dest // n_shard
    nwin = per // window
    counts = np.zeros((n_shard, nwin), np.int64)
    lists = {}
    for s in range(n_shard):
        lo = s * per
        for wi in range(nwin):
            wlo = lo + wi * window
            a = np.searchsorted(row, wlo, side="left")
            b = np.searchsorted(row, wlo + window, side="left")
            lists[(s, wi)] = (row[a:b] - wlo, col[a:b], we[a:b])
            counts[s, wi] = (b - a + 127) // 128
    ncw = np.maximum(counts.max(axis=0), 1)
    C = int(ncw.sum())
    src = np.zeros((n_shard, C, 128), np.int32)
    sel = np.zeros((n_shard, C, 128, 128), np.float32)
    for s in range(n_shard):
        base = 0
        for wi in range(nwin):
            dl, cl, wl = lists[(s, wi)]
            n = len(dl)
            k = int(ncw[wi])
            src[s, base:base + k].reshape(-1)[:n] = cl
            ch = base + np.arange(n) // 128
            ep = np.arange(n) % 128
            sel[s, ch, ep, dl] = wl
            base += k
    return [int(x) for x in ncw], src, sel


def _edge_we(e, n):
    row, col = np.asarray(e[0], np.int64), np.asarray(e[1], np.int64)
    deg = np.bincount(row, minlength=n).astype(np.float32)
    dis = np.where(deg > 0, 1.0 / np.sqrt(np.maximum(deg, 1.0)), 0.0).astype(np.float32)
    return row, col, -(dis[row] * dis[col]).astype(np.float32)


def _sub_edges(row, col, we, pool_idx):
    order = np.argsort(row, kind="stable")
    row, col, we = row[order], col[order], we[order]
    starts = np.searchsorted(row, pool_idx, side="left")
    ends = np.searchsorted(row, pool_idx, side="right")
    nr, ncl, nw = [], [], []
    for i in range(len(pool_idx)):
        s, e = starts[i], ends[i]
        if e > s:
            nr.append(np.full(e - s, i, np.int64))
            ncl.append(col[s:e])
            nw.append(we[s:e])
    return np.concatenate(nr), np.concatenate(ncl), np.concatenate(nw)


def _dense_s(row, col, we, n):
    s = np.zeros((n, n), np.float32)
    np.add.at(s, (row, col), we)
    return s


def _tile_w(w, pack):
    """[K, M] -> [K//(128*pack) * 128, pack*M]: pack K-blocks side by side."""
    k, m = w.shape
    nb = k // 128
    t = w.reshape(nb // pack, pack, 128, m).transpose(0, 2, 1, 3)
    return np.ascontiguousarray(t.reshape((nb // pack) * 128, pack * m))


def _idx_tile(flat):
    """flat int idx list -> [128, len//16] int16 (16-part wrap, x8 replicas)."""
    return np.ascontiguousarray(
        np.tile(flat.astype(np.int16).reshape(-1, 16).T, (8, 1)))


def _sel_tile(sel):
    """[C, 128, 128] f32 -> [128, C*128] bf16 (chunk c at cols c*128..)."""
    C = sel.shape[0]
    return np.ascontiguousarray(
        sel.transpose(1, 0, 2).reshape(128, C * 128)).astype(BF)


def _host_prep(inputs):
    d = {k: np.asarray(v) for k, v in inputs.items()}
    x = d["x"].astype(np.float32)
    l0 = np.asarray(d["l0"], np.int64)
    l1 = np.asarray(d["l1"], np.int64)
    l2 = np.asarray(d["l2"], np.int64)

    X0 = np.ascontiguousarray(x.transpose(1, 0, 2).reshape(N0, 96))
    X0p = np.zeros((N0, 128), np.float32)
    X0p[:, :96] = X0
    X0l0T = np.ascontiguousarray(X0[l0].T)  # [96, 4096]

    r0, c0, w0 = _edge_we(d["e0"], N0)
    ncw_p1, src_p1, sel_p1 = _prep_prop(r0, c0, w0, N0, NCORES)
    r0s, c0s, w0s = _sub_edges(r0, c0, w0, l0)
    ncw_p2, src_p2, sel_p2 = _prep_prop(r0s, c0s, w0s, N1, NCORES)

    r1, c1, w1 = _edge_we(d["e1"], N1)
    ncw_q1, src_q1, sel_q1 = _prep_prop(r1, c1, w1, N1, 1)
    r1s, c1s, w1s = _sub_edges(r1, c1, w1, l1)
    ncw_q2, src_q2, sel_q2 = _prep_prop(r1s, c1s, w1s, N2, 1)

    r2, c2, w2 = _edge_we(d["e2"], N2)
    S2 = _dense_s(r2, c2, w2, N2)
    S2T = _tile_w(np.ascontiguousarray(S2.T), 8).astype(BF)       # [128, 8192]
    S2l2T = _tile_w(np.ascontiguousarray(S2[l2].T), 8).astype(BF)  # [128, 1024]
    P_l2 = np.zeros((N2, 128), np.float32)
    P_l2[l2, np.arange(128)] = 1.0
    P_l2 = _tile_w(P_l2, 8).astype(BF)                             # [128, 1024]

    r3, c3, w3 = _edge_we(d["e3"], N3)
    S3T = np.ascontiguousarray(_dense_s(r3, c3, w3, N3).T).astype(BF)

    def wmod(W):
        return W[0] - W[2], W[1], 2.0 * W[2]

    Wm1 = wmod(d["Wc1"].astype(np.float32))
    Wm = [wmod(d[f"Wc{i}"].astype(np.float32)) for i in (2, 3, 4, 5)]
    eye4 = np.eye(4, dtype=np.float32)

    per_core = []
    for k in range(NCORES):
        m = {}
        m["X0"] = X0p.astype(BF)
        m["X0l0T"] = X0l0T.astype(BF)
        m["identbf"] = np.eye(128, dtype=np.float32).astype(BF)
        m["epsv"] = np.full((128, 1), EPS, np.float32)
        m["l0_idx"] = _idx_tile(l0)
        m["l1_idx"] = _idx_tile(l1)
        for pref, (src, sel) in (
            ("p1", (src_p1[k], sel_p1[k])),
            ("p2", (src_p2[k], sel_p2[k])),
            ("q1", (src_q1[0], sel_q1[0])),
            ("q2", (src_q2[0], sel_q2[0])),
        ):
            m[pref + "_srcw"] = _idx_tile(src.reshape(-1))
            m[pref + "_sel"] = _sel_tile(sel)
        m["S2T"] = S2T
        m["S2l2T"] = S2l2T
        m["P_l2"] = P_l2
        m["S3T"] = S3T
        for t in range(3):
            bw = np.zeros((96, 128), np.float32)
            for j in range(4):
                bg = 4 * k + j
                bw[3 * bg:3 * bg + 3, 32 * j:32 * j + 32] = Wm1[t]
            m[f"bigw0_{t}"] = bw.astype(BF)
        for lev in range(4):
            for t in range(3):
                m[f"bigw{lev + 1}_{t}"] = np.kron(eye4, Wm[lev][t]).astype(BF)
        for lev, nm in ((1, "b1"), (2, "b2"), (3, "b3"), (4, "b4"), (5, "b5")):
            m[f"bias{lev}"] = np.tile(d[nm].astype(np.float32), 4).reshape(128, 1)
        for li in (6, 7, 8):
            W = d[f"W{li}"].astype(np.float32)[:, 512 * k:512 * k + 512]
            # rhs chunks: k2-th 128-row block at cols [512*k2, 512*k2+512)
            m[f"w{li}"] = np.ascontiguousarray(
                W.reshape(32, 128, 512).transpose(1, 0, 2).reshape(128, 32 * 512)
            ).astype(BF)
            m[f"g{li}"] = np.ascontiguousarray(
                d[f"g{li}"].astype(np.float32)[512 * k:512 * k + 512].reshape(4, 128).T)
            m[f"be{li}"] = np.ascontiguousarray(
                d[f"be{li}"].astype(np.float32)[512 * k:512 * k + 512].reshape(4, 128).T)
        W9 = d["W9"].astype(np.float32)[512 * k:512 * k + 512]  # [512, 128]
        m["w9"] = np.ascontiguousarray(
            W9.reshape(4, 128, 128).transpose(1, 0, 2).reshape(128, 512)).astype(BF)
        per_core.append(m)

    meta = {"p1": ncw_p1, "p2": ncw_p2, "q1": ncw_q1, "q2": ncw_q2}
    return per_core, meta


# ---------------------------------------------------------------- device program
def _build_nc(meta, shapes):
    nc = bacc.Bacc("TRN2", target_bir_lowering=False, debug=False, num_devices=NCORES)
    ein = {}
    for name, arr in shapes.items():
        dt = {np.dtype(np.int16): I16, np.dtype(BF): BF16,
              np.dtype(np.float32): F32}[arr.dtype]
        ein[name] = nc.dram_tensor(name, list(arr.shape), dt, kind="ExternalInput")
    out_mu = nc.dram_tensor("mu", [128, 32], F32, kind="ExternalOutput")

    tx1_loc = nc.dram_tensor("tx1_loc", [N0 // 8, 128], BF16)
    tx1_all = nc.dram_tensor("tx1_all", [N0, 128], BF16)
    p2t_loc = nc.dram_tensor("p2t_loc", [96, 512], BF16)
    p2t_all = nc.dram_tensor("p2t_all", [8 * 96, 512], BF16)
    z1_dram = nc.dram_tensor("z1_dram", [N1, 128], BF16)
    t1l1_dram = nc.dram_tensor("t1l1_dram", [N1, 128], BF16)
    x_loc = nc.dram_tensor("x_loc", [4, 4096], BF16)
    x_all = nc.dram_tensor("x_all", [32, 4096], BF16)
    h6_loc = nc.dram_tensor("h6_loc", [128, 128], BF16)
    h6_all = nc.dram_tensor("h6_all", [1024, 128], BF16)
    h7_loc = nc.dram_tensor("h7_loc", [128, 128], BF16)
    h7_all = nc.dram_tensor("h7_all", [1024, 128], BF16)
    p9_loc = nc.dram_tensor("p9_loc", [32, 128], F32)
    p9_all = nc.dram_tensor("p9_all", [256, 128], F32)

    with tile.TileContext(nc) as tc:
        with (
            tc.tile_pool(name="const", bufs=1) as cpool,
            tc.tile_pool(name="big", bufs=1) as bigpool,
            tc.tile_pool(name="work", bufs=3) as wpool,
            tc.tile_pool(name="wload", bufs=2) as wlpool,
            tc.tile_pool(name="psA", bufs=3, space="PSUM") as ppool,
            tc.tile_pool(name="psT", bufs=2, space="PSUM") as tpool,
            tc.tile_pool(name="psB", bufs=1, space="PSUM") as apool,
        ):
            ident = cpool.tile([128, 128], BF16, tag="identbf", name="identbf")
            nc.sync.dma_start(out=ident[:], in_=ein["identbf"][:, :])
            eps_t = cpool.tile([128, 1], F32, tag="epsv", name="epsv")
            nc.sync.dma_start(out=eps_t[:], in_=ein["epsv"][:, :])

            def load_const(name, dt=BF16):
                t = cpool.tile(list(shapes[name].shape), dt, tag=name)
                nc.sync.dma_start(out=t[:], in_=ein[name][:, :])
                return t

            GRP = 16

            def grp_gather(idx_sb, g0, gc, gather_src):
                zb = wpool.tile([128, GRP * 128], BF16, tag="zb", name="zb", bufs=3)
                nc.gpsimd.dma_gather(
                    out_ap=zb[:, :gc * 128].rearrange("p (c e) -> p c e", e=128),
                    in_ap=gather_src[:, :],
                    idxs_ap=idx_sb[:, g0 * 8:(g0 + gc) * 8],
                    num_idxs=gc * 128, num_idxs_reg=gc * 128, elem_size=128,
                    single_packet=False)
                return zb

            def grp_sel(pref, g0, gc):
                sl = wpool.tile([128, GRP * 128], BF16, tag="selg", name="selg", bufs=3)
                nc.sync.dma_start(out=sl[:, :gc * 128],
                                  in_=ein[pref + "_sel"][:, g0 * 128:(g0 + gc) * 128])
                return sl

            def transp(src_ap, dst_ap, dt=BF16):
                p, f = src_ap.shape
                ps = tpool.tile([128, 128], dt, tag="tp", name="tp")
                nc.tensor.transpose(out=ps[:f, :p], in_=src_ap, identity=ident[:p, :p])
                nc.scalar.activation(out=dst_ap, in_=ps[:f, :p], func=AF.Copy)

            def load_chunks(pref, C):
                s = cpool.tile([128, C * 8], I16, tag=pref + "s", name=pref + "s")
                nc.sync.dma_start(out=s[:], in_=ein[pref + "_srcw"][:, :])
                return s

            def prop_nodemajor(ncw, pref, gather_src, D, evac):
                C = sum(ncw)
                s = load_chunks(pref, C)
                zbs, sls = {}, {}
                for g0 in range(0, C, GRP):
                    gc = min(GRP, C - g0)
                    zbs[g0] = grp_gather(s, g0, gc, gather_src)
                    sls[g0] = grp_sel(pref, g0, gc)
                base = 0
                for wi, nch in enumerate(ncw):
                    ps = ppool.tile([128, 512], F32, tag="ps", name="ps")
                    for c in range(nch):
                        cc = base + c
                        g0 = (cc // GRP) * GRP
                        lo = (cc % GRP) * 128
                        nc.tensor.matmul(out=ps[:, :D], lhsT=sls[g0][:, lo:lo + 128],
                                         rhs=zbs[g0][:, lo:lo + D],
                                         start=(c == 0), stop=(c == nch - 1))
                    evac(wi, ps[:, :D])
                    base += nch

            def gather_T(idx_t, g0, gc, gather_src, D, outT):
                zb = grp_gather(idx_t, g0, gc, gather_src)
                for ci in range(gc):
                    transp(zb[:, ci * 128:ci * 128 + D],
                           outT[:, ci * 128:(ci + 1) * 128])

            def einsum_win(bigw, taps, Din, width, out_ap, func, bias_ap):
                ps = ppool.tile([128, 512], F32, tag="ps", name="ps")
                for t in range(3):
                    nc.tensor.matmul(out=ps[:, :width], lhsT=bigw[t][:Din, :],
                                     rhs=taps[t], start=(t == 0), stop=(t == 2))
                f2 = AF.Identity if func == AF.Copy else func
                nc.scalar.activation(out=out_ap, in_=ps[:, :width], func=f2, bias=bias_ap)

            # ================= LEVEL 0 =================
            with nc.named_scope("l0_prop1"):
                def evac_p1(wi, ps_ap):
                    t = wpool.tile([128, 96], BF16, tag="ev", name="ev", bufs=6)
                    nc.scalar.activation(out=t[:], in_=ps_ap, func=AF.Copy)
                    nc.sync.dma_start(out=tx1_loc[wi * 128:(wi + 1) * 128, :96], in_=t[:])
                prop_nodemajor(meta["p1"], "p1", ein["X0"], 96, evac_p1)
            with nc.named_scope("ag1"):
                nc.gpsimd.collective_compute(
                    "AllGather", ALU.bypass, replica_groups=RG,
                    ins=[tx1_loc.ap().opt()], outs=[tx1_all.ap().opt()])

            with nc.named_scope("l0_prop2"):
                C2 = sum(meta["p2"])
                s2c = load_chunks("p2", C2)
                zbs2, sls2 = {}, {}
                for g0 in range(0, C2, GRP):
                    gc = min(GRP, C2 - g0)
                    zbs2[g0] = grp_gather(s2c, g0, gc, tx1_all)
                    sls2[g0] = grp_sel("p2", g0, gc)
                p2t_sb = bigpool.tile([96, 512], BF16, tag="p2t_sb", name="p2t_sb")
                base = 0
                for wi, nch in enumerate(meta["p2"]):
                    ps = ppool.tile([128, 512], F32, tag="ps", name="ps")
                    for c in range(nch):
                        cc = base + c
                        g0 = (cc // GRP) * GRP
                        lo = (cc % GRP) * 128
                        nc.tensor.matmul(out=ps[:96, :128],
                                         lhsT=zbs2[g0][:, lo:lo + 96],
                                         rhs=sls2[g0][:, lo:lo + 128],
                                         start=(c == 0), stop=(c == nch - 1))
                    nc.scalar.activation(out=p2t_sb[:, wi * 128:(wi + 1) * 128],
                                         in_=ps[:96, :128], func=AF.Copy)
                    base += nch
                nc.sync.dma_start(out=p2t_loc[:, :], in_=p2t_sb[:])
            with nc.named_scope("ag2"):
                nc.gpsimd.collective_compute(
                    "AllGather", ALU.bypass, replica_groups=RG,
                    ins=[p2t_loc.ap().opt()], outs=[p2t_all.ap().opt()])

            with nc.named_scope("l0_einsum"):
                l0i = cpool.tile([128, 32 * 8], I16, tag="l0i", name="l0i")
                nc.sync.dma_start(out=l0i[:], in_=ein["l0_idx"][:, :])
                bw0 = [load_const(f"bigw0_{t}") for t in range(3)]
                bias1 = load_const("bias1", F32)
                for w in range(8):
                    g0w = wpool.tile([96, 512], BF16, tag="g0w", name="g0w")
                    nc.sync.dma_start(out=g0w[:], in_=ein["X0l0T"][:, 512 * w:512 * (w + 1)])
                    g1w = wpool.tile([96, 512], BF16, tag="g1w", name="g1w")
                    gather_T(l0i, 4 * w, 4, tx1_all, 96, g1w)
                    p2w = wpool.tile([96, 512], BF16, tag="p2w", name="p2w")
                    nc.sync.dma_start(out=p2w[:], in_=p2t_all[96 * w:96 * (w + 1), :])
                    z1Tw = wpool.tile([128, 512], BF16, tag="z1Tw", name="z1Tw")
                    einsum_win(bw0, [g0w[:], g1w[:], p2w[:]], 96, 512,
                               z1Tw[:], AF.Copy, bias1[:, 0:1])
                    for c in range(4):
                        t = wpool.tile([128, 128], BF16, tag="z1nc", name="z1nc")
                        transp(z1Tw[:, c * 128:(c + 1) * 128], t[:])
                        r = w * 512 + c * 128
                        nc.sync.dma_start(out=z1_dram[r:r + 128, :], in_=t[:])

            # ================= LEVEL 1 =================
            with nc.named_scope("l1_prop1"):
                def evac_q1(wi, ps_ap):
                    t = wpool.tile([128, 128], BF16, tag="ev", name="ev", bufs=6)
                    nc.scalar.activation(out=t[:], in_=ps_ap, func=AF.Copy)
                    nc.sync.dma_start(out=t1l1_dram[wi * 128:(wi + 1) * 128, :], in_=t[:])
                prop_nodemajor(meta["q1"], "q1", z1_dram, 128, evac_q1)

            p2n_l1 = bigpool.tile([128, 8 * 128], BF16, tag="p2n_l1", name="p2n_l1")
            with nc.named_scope("l1_prop2"):
                def evac_q2(wi, ps_ap):
                    nc.scalar.activation(out=p2n_l1[:, wi * 128:(wi + 1) * 128],
                                         in_=ps_ap, func=AF.Copy)
                prop_nodemajor(meta["q2"], "q2", t1l1_dram, 128, evac_q2)

            z2n = bigpool.tile([128, 8 * 128], BF16, tag="z2n", name="z2n")
            with nc.named_scope("l1_einsum"):
                l1i = cpool.tile([128, 8 * 8], I16, tag="l1i", name="l1i")
                nc.sync.dma_start(out=l1i[:], in_=ein["l1_idx"][:, :])
                z1l1T = bigpool.tile([128, 1024], BF16, tag="z1l1T", name="z1l1T")
                gather_T(l1i, 0, 8, z1_dram, 128, z1l1T)
                t1l1T = bigpool.tile([128, 1024], BF16, tag="t1l1T", name="t1l1T")
                gather_T(l1i, 0, 8, t1l1_dram, 128, t1l1T)
                p2l1T = bigpool.tile([128, 1024], BF16, tag="p2l1T", name="p2l1T")
                for c in range(8):
                    transp(p2n_l1[:, c * 128:(c + 1) * 128], p2l1T[:, c * 128:(c + 1) * 128])
                bw1 = [load_const(f"bigw1_{t}") for t in range(3)]
                bias2 = load_const("bias2", F32)
                z2T = bigpool.tile([128, 1024], BF16, tag="z2T", name="z2T")
                for w in range(2):
                    einsum_win(bw1, [z1l1T[:, 512 * w:512 * (w + 1)],
                                     t1l1T[:, 512 * w:512 * (w + 1)],
                                     p2l1T[:, 512 * w:512 * (w + 1)]],
                               128, 512, z2T[:, 512 * w:512 * (w + 1)], AF.Tanh, bias2[:, 0:1])
                for c in range(8):
                    transp(z2T[:, c * 128:(c + 1) * 128], z2n[:, c * 128:(c + 1) * 128])

            # ================= LEVEL 2 (dense) =================
            with nc.named_scope("l2"):
                t1_l2 = bigpool.tile([128, 8 * 128], BF16, tag="t1_l2", name="t1_l2")
                for half in range(2):
                    s2t = wlpool.tile([128, 4096], BF16, tag="wld", name="wld")
                    nc.sync.dma_start(out=s2t[:], in_=ein["S2T"][:, 4096 * half:4096 * (half + 1)])
                    for dc in range(8):
                        ps = ppool.tile([128, 512], F32, tag="ps", name="ps")
                        for kk in range(4):
                            kc = half * 4 + kk
                            nc.tensor.matmul(
                                out=ps[:, :128],
                                lhsT=s2t[:, kk * 1024 + dc * 128: kk * 1024 + dc * 128 + 128],
                                rhs=z2n[:, kc * 128:(kc + 1) * 128],
                                start=(kk == 0), stop=(kk == 3))
                        if half == 0:
                            nc.scalar.activation(out=t1_l2[:, dc * 128:(dc + 1) * 128],
                                                 in_=ps[:, :128], func=AF.Copy)
                        else:
                            nc.vector.tensor_add(t1_l2[:, dc * 128:(dc + 1) * 128],
                                                 t1_l2[:, dc * 128:(dc + 1) * 128],
                                                 ps[:, :128])
                s2l2 = cpool.tile([128, 1024], BF16, tag="s2l2", name="s2l2")
                nc.sync.dma_start(out=s2l2[:], in_=ein["S2l2T"][:, :])
                ps = ppool.tile([128, 512], F32, tag="ps", name="ps")
                for kc in range(8):
                    nc.tensor.matmul(out=ps[:, :128], lhsT=s2l2[:, kc * 128:(kc + 1) * 128],
                                     rhs=t1_l2[:, kc * 128:(kc + 1) * 128],
                                     start=(kc == 0), stop=(kc == 7))
                p2n_l2 = wpool.tile([128, 128], BF16, tag="p2n_l2", name="p2n_l2")
                nc.scalar.activation(out=p2n_l2[:], in_=ps[:, :128], func=AF.Copy)
                pl2 = cpool.tile([128, 1024], BF16, tag="pl2", name="pl2")
                nc.sync.dma_start(out=pl2[:], in_=ein["P_l2"][:, :])
                z2l2T = wpool.tile([128, 128], BF16, tag="z2l2T", name="z2l2T")
                psg = ppool.tile([128, 512], F32, tag="ps", name="ps")
                for kc in range(8):
                    nc.tensor.matmul(out=psg[:, :128], lhsT=z2n[:, kc * 128:(kc + 1) * 128],
                                     rhs=pl2[:, kc * 128:(kc + 1) * 128],
                                     start=(kc == 0), stop=(kc == 7))
                nc.scalar.activation(out=z2l2T[:], in_=psg[:, :128], func=AF.Copy)
                t1l2T = wpool.tile([128, 128], BF16, tag="t1l2T", name="t1l2T")
                psg2 = ppool.tile([128, 512], F32, tag="ps", name="ps")
                for kc in range(8):
                    nc.tensor.matmul(out=psg2[:, :128], lhsT=t1_l2[:, kc * 128:(kc + 1) * 128],
                                     rhs=pl2[:, kc * 128:(kc + 1) * 128],
                                     start=(kc == 0), stop=(kc == 7))
                nc.scalar.activation(out=t1l2T[:], in_=psg2[:, :128], func=AF.Copy)
                p2l2T = wpool.tile([128, 128], BF16, tag="p2l2T", name="p2l2T")
                transp(p2n_l2[:], p2l2T[:])
                bw2 = [load_const(f"bigw2_{t}") for t in range(3)]
                bias3 = load_const("bias3", F32)
                z3T = wpool.tile([128, 128], BF16, tag="z3T", name="z3T")
                einsum_win(bw2, [z2l2T[:], t1l2T[:], p2l2T[:]], 128, 128,
                           z3T[:], AF.Tanh, bias3[:, 0:1])
                z3n = wpool.tile([128, 128], BF16, tag="z3n", name="z3n")
                transp(z3T[:], z3n[:])

            # ================= LEVEL 3 =================
            with nc.named_scope("l3"):
                s3t = cpool.tile([128, 128], BF16, tag="s3t", name="s3t")
                nc.sync.dma_start(out=s3t[:], in_=ein["S3T"][:, :])
                bias4 = load_const("bias4", F32)
                bias5 = load_const("bias5", F32)

                def conv_l3(zn, zT, bw_pref, bias_t, func, keep):
                    t1T = wpool.tile([128, 128], BF16, tag=keep + "t1T", name=keep + "t1T")
                    ps = ppool.tile([128, 512], F32, tag="ps", name="ps")
                    nc.tensor.matmul(out=ps[:, :128], lhsT=zn, rhs=s3t[:], start=True, stop=True)
                    nc.scalar.activation(out=t1T[:], in_=ps[:, :128], func=AF.Copy)
                    t1n_ = wpool.tile([128, 128], BF16, tag=keep + "t1n", name=keep + "t1n")
                    transp(t1T[:], t1n_[:])
                    p2T_ = wpool.tile([128, 128], BF16, tag=keep + "p2T", name=keep + "p2T")
                    ps2 = ppool.tile([128, 512], F32, tag="ps", name="ps")
                    nc.tensor.matmul(out=ps2[:, :128], lhsT=t1n_[:], rhs=s3t[:], start=True, stop=True)
                    nc.scalar.activation(out=p2T_[:], in_=ps2[:, :128], func=AF.Copy)
                    bw = [load_const(f"{bw_pref}_{t}") for t in range(3)]
                    outT = wpool.tile([128, 128], BF16, tag=keep + "oT", name=keep + "oT")
                    einsum_win(bw, [zT, t1T[:], p2T_[:]], 128, 128, outT[:], func, bias_t[:, 0:1])
                    outn = wpool.tile([128, 128], BF16, tag=keep + "on", name=keep + "on")
                    transp(outT[:], outn[:])
                    return outn, outT

                z4n, z4T = conv_l3(z3n[:], z3T[:], "bigw3", bias4, AF.Tanh, "c4")
                o5n, o5T = conv_l3(z4n[:], z4T[:], "bigw4", bias5, AF.Copy, "c5")

            # ================= MLP input assembly =================
            with nc.named_scope("mlp_in"):
                # x_loc[b, n*32+c] = o5n[n, 32b+c] for this core's 4 batches
                nc.sync.dma_start(
                    out=x_loc.ap().rearrange("b (n c) -> n b c", c=32),
                    in_=o5n[:].rearrange("n (b c) -> n b c", c=32))
                nc.gpsimd.collective_compute(
                    "AllGather", ALU.bypass, replica_groups=RG,
                    ins=[x_loc.ap().opt()], outs=[x_all.ap().opt()])
                xT_sb = bigpool.tile([32, 4096], BF16, tag="xT_sb", name="xT_sb")
                nc.sync.dma_start(out=xT_sb[:], in_=x_all[:, :])
                act6 = bigpool.tile([128, 1024], BF16, tag="act6", name="act6")
                for i in range(32):
                    transp(xT_sb[:, 128 * i:128 * (i + 1)], act6[:, 32 * i:32 * i + 32])

            # ================= MLP =================
            def mlp_layer(li, act_sb, out_sb):
                g_t = load_const(f"g{li}", F32)
                be_t = load_const(f"be{li}", F32)
                wt = wlpool.tile([128, 32 * 512], BF16, tag="wld", name="wld")
                nc.sync.dma_start(out=wt[:, :8192], in_=ein[f"w{li}"][:, :8192])
                nc.sync.dma_start(out=wt[:, 8192:], in_=ein[f"w{li}"][:, 8192:])
                acc = apool.tile([128, 512], F32, tag="acc", name="acc")
                for k2 in range(32):
                    nc.tensor.matmul(out=acc[:32, :], lhsT=act_sb[:, 32 * k2:32 * k2 + 32],
                                     rhs=wt[:, 512 * k2:512 * (k2 + 1)],
                                     start=(k2 == 0), stop=(k2 == 31))
                hb = wpool.tile([32, 512], BF16, tag="hb", name="hb")
                nc.scalar.activation(out=hb[:], in_=acc[:32, :], func=AF.Copy)
                for c in range(4):
                    hc = wpool.tile([128, 32], BF16, tag="hc", name="hc")
                    transp(hb[:, 128 * c:128 * (c + 1)], hc[:])
                    s1 = wpool.tile([128, 1], F32, tag="b_s1", name="b_s1")
                    nc.vector.tensor_reduce(out=s1[:], in_=hc[:], axis=AX.X, op=ALU.add)
                    mu_ = wpool.tile([128, 1], F32, tag="b_mu", name="b_mu")
                    nc.vector.tensor_scalar_mul(mu_[:], s1[:], 1.0 / 32.0)
                    sq = wpool.tile([128, 32], F32, tag="b_sq", name="b_sq")
                    nc.vector.tensor_mul(sq[:], hc[:], hc[:])
                    s2_ = wpool.tile([128, 1], F32, tag="b_s2", name="b_s2")
                    nc.vector.tensor_reduce(out=s2_[:], in_=sq[:], axis=AX.X, op=ALU.add)
                    var = wpool.tile([128, 1], F32, tag="b_var", name="b_var")
                    nc.vector.scalar_tensor_tensor(out=var[:], in0=mu_[:], scalar=-1.0,
                                                   in1=mu_[:], op0=ALU.mult, op1=ALU.mult)
                    nc.vector.scalar_tensor_tensor(out=var[:], in0=s2_[:], scalar=1.0 / 32.0,
                                                   in1=var[:], op0=ALU.mult, op1=ALU.add)
                    sd = wpool.tile([128, 1], F32, tag="b_sd", name="b_sd")
                    nc.scalar.activation(out=sd[:], in_=var[:], func=AF.Sqrt, bias=eps_t[:, 0:1])
                    rs = wpool.tile([128, 1], F32, tag="b_rs", name="b_rs")
                    nc.vector.reciprocal(rs[:], sd[:])
                    a_ = wpool.tile([128, 1], F32, tag="b_a", name="b_a")
                    nc.vector.tensor_mul(a_[:], rs[:], g_t[:, c:c + 1])
                    sh = wpool.tile([128, 1], F32, tag="b_sh", name="b_sh")
                    nc.vector.scalar_tensor_tensor(out=sh[:], in0=mu_[:], scalar=-1.0,
                                                   in1=a_[:], op0=ALU.mult, op1=ALU.mult)
                    nc.vector.tensor_add(sh[:], sh[:], be_t[:, c:c + 1])
                    nc.scalar.activation(out=out_sb[:, 32 * c:32 * c + 32], in_=hc[:],
                                         func=AF.Relu, scale=a_[:, 0:1], bias=sh[:, 0:1])

            with nc.named_scope("mlp6"):
                h6 = bigpool.tile([128, 128], BF16, tag="h6sb", name="h6sb")
                mlp_layer(6, act6, h6)
                nc.sync.dma_start(out=h6_loc.ap(), in_=h6[:])
                nc.gpsimd.collective_compute(
                    "AllGather", ALU.bypass, replica_groups=RG,
                    ins=[h6_loc.ap().opt()], outs=[h6_all.ap().opt()])
            with nc.named_scope("mlp7"):
                act7 = bigpool.tile([128, 1024], BF16, tag="act7", name="act7")
                for r in range(8):
                    nc.sync.dma_start(out=act7[:, 128 * r:128 * (r + 1)],
                                      in_=h6_all[128 * r:128 * (r + 1), :])
                h7 = bigpool.tile([128, 128], BF16, tag="h7sb", name="h7sb")
                mlp_layer(7, act7, h7)
                nc.sync.dma_start(out=h7_loc.ap(), in_=h7[:])
                nc.gpsimd.collective_compute(
                    "AllGather", ALU.bypass, replica_groups=RG,
                    ins=[h7_loc.ap().opt()], outs=[h7_all.ap().opt()])
            with nc.named_scope("mlp8"):
                act8 = bigpool.tile([128, 1024], BF16, tag="act8", name="act8")
                for r in range(8):
                    nc.sync.dma_start(out=act8[:, 128 * r:128 * (r + 1)],
                                      in_=h7_all[128 * r:128 * (r + 1), :])
                h8 = bigpool.tile([128, 128], BF16, tag="h8sb", name="h8sb")
                mlp_layer(8, act8, h8)

            with nc.named_scope("mlp9"):
                w9t = cpool.tile([128, 512], BF16, tag="w9t", name="w9t")
                nc.sync.dma_start(out=w9t[:], in_=ein["w9"][:, :])
                acc9 = apool.tile([128, 512], F32, tag="acc", name="acc9")
                for c in range(4):
                    nc.tensor.matmul(out=acc9[:32, :128], lhsT=h8[:, 32 * c:32 * c + 32],
                                     rhs=w9t[:, 128 * c:128 * (c + 1)],
                                     start=(c == 0), stop=(c == 3))
                p9sb = wpool.tile([32, 128], F32, tag="p9sb", name="p9sb")
                nc.scalar.activation(out=p9sb[:], in_=acc9[:32, :128], func=AF.Copy)
                nc.sync.dma_start(out=p9_loc.ap(), in_=p9sb[:])
                nc.gpsimd.collective_compute(
                    "AllGather", ALU.bypass, replica_groups=RG,
                    ins=[p9_loc.ap().opt()], outs=[p9_all.ap().opt()])
                tot = wpool.tile([32, 128], F32, tag="f_tot", name="f_tot")
                nc.sync.dma_start(out=tot[:], in_=p9_all[0:32, :])
                for k in range(1, 8):
                    pk = wpool.tile([32, 128], F32, tag="f_pk", name="f_pk")
                    nc.sync.dma_start(out=pk[:], in_=p9_all[32 * k:32 * (k + 1), :])
                    nc.vector.tensor_add(tot[:], tot[:], pk[:])
                totT = wpool.tile([128, 32], F32, tag="f_totT", name="f_totT")
                pst = ppool.tile([128, 512], F32, tag="ps", name="pst")
                identf = cpool.tile([32, 32], F32, tag="identf", name="identf")
                nc.scalar.activation(out=identf[:], in_=ident[:32, :32], func=AF.Copy)
                nc.tensor.transpose(out=pst[:128, :32], in_=tot[:], identity=identf[:])
                nc.scalar.activation(out=totT[:], in_=pst[:128, :32], func=AF.Copy)
                s1 = wpool.tile([128, 1], F32, tag="f_s1", name="f_s1")
                nc.vector.tensor_reduce(out=s1[:], in_=totT[:], axis=AX.X, op=ALU.add)
                mu_ = wpool.tile([128, 1], F32, tag="f_mu", name="f_mu")
                nc.vector.tensor_scalar_mul(mu_[:], s1[:], 1.0 / 32.0)
                sq = wpool.tile([128, 32], F32, tag="f_sq", name="f_sq")
                nc.vector.tensor_mul(sq[:], totT[:], totT[:])
                s2_ = wpool.tile([128, 1], F32, tag="f_s2", name="f_s2")
                nc.vector.tensor_reduce(out=s2_[:], in_=sq[:], axis=AX.X, op=ALU.add)
                var = wpool.tile([128, 1], F32, tag="f_var", name="f_var")
                nc.vector.scalar_tensor_tensor(out=var[:], in0=mu_[:], scalar=-1.0,
                                               in1=mu_[:], op0=ALU.mult, op1=ALU.mult)
                nc.vector.scalar_tensor_tensor(out=var[:], in0=s2_[:], scalar=1.0 / 32.0,
                                               in1=var[:], op0=ALU.mult, op1=ALU.add)
                sdf = wpool.tile([128, 1], F32, tag="f_sd", name="f_sd")
                nc.scalar.activation(out=sdf[:], in_=var[:], func=AF.Sqrt, bias=eps_t[:, 0:1])
                rs = wpool.tile([128, 1], F32, tag="f_rs", name="f_rs")
                nc.vector.reciprocal(rs[:], sdf[:])
                neg = wpool.tile([128, 1], F32, tag="f_neg", name="f_neg")
                nc.vector.scalar_tensor_tensor(out=neg[:], in0=mu_[:], scalar=-1.0,
                                               in1=rs[:], op0=ALU.mult, op1=ALU.mult)
                outt = wpool.tile([128, 32], F32, tag="f_out", name="f_out")
                nc.scalar.activation(out=outt[:], in_=totT[:], func=AF.Identity,
                                     scale=rs[:, 0:1], bias=neg[:, 0:1])
                nc.sync.dma_start(out=out_mu[:, :], in_=outt[:])

    nc.compile()
    return nc


# ---------------------------------------------------------------- entry point
def kernel(**inputs) -> np.ndarray:
    per_core, meta = _host_prep(inputs)
    if "prog" not in _CACHE:
        _CACHE["prog"] = _build_nc(meta, per_core[0])
    nc = _CACHE["prog"]
    res = bass_utils.run_bass_kernel_spmd(nc, per_core, core_ids=list(range(NCORES)))
    return np.ascontiguousarray(res.results[0]["mu"].T)


# revision 17
# speedup vs baseline: 1.7589x; 1.3524x over previous
"""Trainium2 Bass kernel for nn_Encoder_base (5x ChebConv GNN + pool + MLP).

Distribution over 8 NeuronCores (all matmuls fp16, fp32 PSUM):
  - level 0: edge-dest-sharded props over 96 = 32*3 batch-features.
    Gather sources are the INPUT x, so edge-major copies (X0g) are built on
    the host -> plain contiguous DMA, no indirect gathers. Tap1 (Tx1[l0]) is
    computed as a second restricted prop on X0g (same sel as prop2).
  - level 1: edge-dest-sharded props over the full 1024 = 32*32 features
    (2KB gather rows -> 8x fewer indirect-DMA descriptors than
    batch-sharding), chunked AllGathers of z1/t1 overlap compute.
  - levels 2-3 + einsums: batch-sharded (4 batches/core, 128 featcols),
    dense-S matmuls, block-diagonal channel mixes in feature-major layout.
  - MLP: output-feature sharded (512 cols of W6/7/8, 512 rows of W9 per
    core); activations [128k,32] are the stationary lhsT, W streams as rhs;
    BatchNorm per-feature after a PE transpose; activations AllGathered.
Selection matrices are precomputed on the host and streamed from DRAM.
"""
import numpy as np
import concourse.bass as bass
import concourse.bacc as bacc
import concourse.tile as tile
from concourse import mybir, bass_utils

F32 = mybir.dt.float32
F16 = mybir.dt.float16
I16 = mybir.dt.int16
AF = mybir.ActivationFunctionType
ALU = mybir.AluOpType
AX = mybir.AxisListType
RG = [list(range(8))]
NCORES = 8
N0, N1, N2, N3 = 16384, 4096, 1024, 128
EPS = 1e-5
H16 = np.float16

_CACHE = {}


# ---------------------------------------------------------------- host prep
def _prep_prop(row, col, we, n_dest, n_shard):
    """Sorted-by-dest edges -> 128-dest windows, 128-edge chunks, padded so
    chunk counts per window match across shards (one SPMD program).
    Emits per-chunk selection matrices sel[chunk, edge_local, dst_local]."""
    window = 128
    order = np.argsort(row, kind="stable")
    row, col, we = row[order], col[order], we[order]
    per = n_dest // n_shard
    nwin = per // window
    counts = np.zeros((n_shard, nwin), np.int64)
    lists = {}
    for s in range(n_shard):
        lo = s * per
        for wi in range(nwin):
            wlo = lo + wi * window
            a = np.searchsorted(row, wlo, side="left")
            b = np.searchsorted(row, wlo + window, side="left")
            lists[(s, wi)] = (row[a:b] - wlo, col[a:b], we[a:b])
            counts[s, wi] = (b - a + 127) // 128
    ncw = np.maximum(counts.max(axis=0), 1)
    C = int(ncw.sum())
    src = np.zeros((n_shard, C, 128), np.int64)
    sel = np.zeros((n_shard, C, 128, 128), np.float32)
    for s in range(n_shard):
        base = 0
        for wi in range(nwin):
            dl, cl, wl = lists[(s, wi)]
            n = len(dl)
            k = int(ncw[wi])
            src[s, base:base + k].reshape(-1)[:n] = cl
            ch = base + np.arange(n) // 128
            ep = np.arange(n) % 128
            sel[s, ch, ep, dl] = wl
            base += k
    return [int(x) for x in ncw], src, sel


def _edge_we(e, n):
    row, col = np.asarray(e[0], np.int64), np.asarray(e[1], np.int64)
    deg = np.bincount(row, minlength=n).astype(np.float32)
    dis = np.where(deg > 0, 1.0 / np.sqrt(np.maximum(deg, 1.0)), 0.0).astype(np.float32)
    return row, col, -(dis[row] * dis[col]).astype(np.float32)


def _sub_edges(row, col, we, pool_idx):
    order = np.argsort(row, kind="stable")
    row, col, we = row[order], col[order], we[order]
    starts = np.searchsorted(row, pool_idx, side="left")
    ends = np.searchsorted(row, pool_idx, side="right")
    nr, ncl, nw = [], [], []
    for i in range(len(pool_idx)):
        s, e = starts[i], ends[i]
        if e > s:
            nr.append(np.full(e - s, i, np.int64))
            ncl.append(col[s:e])
            nw.append(we[s:e])
    return np.concatenate(nr), np.concatenate(ncl), np.concatenate(nw)


def _dense_s(row, col, we, n):
    s = np.zeros((n, n), np.float32)
    np.add.at(s, (row, col), we)
    return s


def _tile_w(w, pack):
    """[K, M] -> [K//(128*pack) * 128, pack*M]: pack K-blocks side by side."""
    k, m = w.shape
    nb = k // 128
    t = w.reshape(nb // pack, pack, 128, m).transpose(0, 2, 1, 3)
    return np.ascontiguousarray(t.reshape((nb // pack) * 128, pack * m))


def _idx_tile(flat):
    """flat int idx list -> [128, len//16] int16 (16-part wrap, x8 replicas)."""
    return np.ascontiguousarray(
        np.tile(flat.astype(np.int16).reshape(-1, 16).T, (8, 1)))


def _chunk_tile(arr3):
    """[C, 128, W] -> [128, C*W] (chunk c at cols c*W..)."""
    C, _, W = arr3.shape
    return np.ascontiguousarray(
        arr3.transpose(1, 0, 2).reshape(128, C * W)).astype(H16)


def _remap_tx1(n):
    """tx1 global row -> row in chunk-AG layout (4 AG chunks of 4 windows)."""
    r, q = n // 2048, n % 2048
    w, o = q // 128, q % 128
    return 4096 * (w // 4) + 512 * r + 128 * (w % 4) + o


def _remap_t1(n):
    """t1 global row -> row in chunk-AG layout (4 AG chunks of 1 window)."""
    r, q = n // 512, n % 512
    w, o = q // 128, q % 128
    return 1024 * w + 128 * r + o


def _host_prep(inputs):
    d = {k: np.asarray(v) for k, v in inputs.items()}
    x = d["x"].astype(np.float32)
    l0 = np.asarray(d["l0"], np.int64)
    l1 = np.asarray(d["l1"], np.int64)
    l2 = np.asarray(d["l2"], np.int64)

    X0 = np.ascontiguousarray(x.transpose(1, 0, 2).reshape(N0, 96))
    X0p = np.zeros((N0, 128), np.float32)
    X0p[:, :96] = X0
    X0l0T = np.ascontiguousarray(X0[l0].T)  # [96, 4096]

    r0, c0, w0 = _edge_we(d["e0"], N0)
    ncw_p1, src_p1, sel_p1 = _prep_prop(r0, c0, w0, N0, NCORES)
    r0s, c0s, w0s = _sub_edges(r0, c0, w0, l0)
    ncw_p2, src_p2, sel_p2 = _prep_prop(r0s, c0s, w0s, N1, NCORES)

    r1, c1, w1 = _edge_we(d["e1"], N1)
    ncw_q1, src_q1, sel_q1 = _prep_prop(r1, c1, w1, N1, NCORES)
    r1s, c1s, w1s = _sub_edges(r1, c1, w1, l1)
    ncw_q2, src_q2, sel_q2 = _prep_prop(r1s, c1s, w1s, N2, NCORES)

    r2, c2, w2 = _edge_we(d["e2"], N2)
    S2 = _dense_s(r2, c2, w2, N2)
    S2T = _tile_w(np.ascontiguousarray(S2.T), 8).astype(H16)       # [128, 8192]
    S2l2T = _tile_w(np.ascontiguousarray(S2[l2].T), 8).astype(H16)  # [128, 1024]
    P_l2 = np.zeros((N2, 128), np.float32)
    P_l2[l2, np.arange(128)] = 1.0
    P_l2 = _tile_w(P_l2, 8).astype(H16)                             # [128, 1024]

    r3, c3, w3 = _edge_we(d["e3"], N3)
    S3T = np.ascontiguousarray(_dense_s(r3, c3, w3, N3).T).astype(H16)

    def wmod(W):
        return W[0] - W[2], W[1], 2.0 * W[2]

    Wm1 = wmod(d["Wc1"].astype(np.float32))
    Wm = [wmod(d[f"Wc{i}"].astype(np.float32)) for i in (2, 3, 4, 5)]
    eye4 = np.eye(4, dtype=np.float32)

    per_core = []
    for k in range(NCORES):
        m = {}
        m["identbf"] = np.eye(128, dtype=np.float32).astype(H16)
        m["epsv"] = np.full((128, 1), EPS, np.float32)
        # host-pregathered edge-major X0 rows (p1 and the shared p2/p1l0 set)
        m["X0g_p1"] = _chunk_tile(X0p[src_p1[k]].astype(H16))  # [128, C1*128]
        m["X0g_p2"] = _chunk_tile(X0p[src_p2[k]].astype(H16))  # [128, C2*128]
        m["X0l0T"] = X0l0T.astype(H16)
        for pref, (src, sel) in (
            ("p1", (None, sel_p1[k])),
            ("p2", (_remap_tx1(src_p2[k]), sel_p2[k])),
            ("q1", (src_q1[k], sel_q1[k])),
            ("q2", (_remap_t1(src_q2[k]), sel_q2[k])),
        ):
            if src is not None:
                m[pref + "_srcw"] = _idx_tile(src.reshape(-1))
            m[pref + "_sel"] = _chunk_tile(sel)
        m["l1z_idx"] = _idx_tile(l1)              # into z1_loc (true order)
        # into t1_all flattened [4096*8, 128]: wide row remapped, sub-row = core
        m["l1t_idx"] = _idx_tile(_remap_t1(l1) * 8 + k)
        # einsum tap: p2_all[n, own 128 cols] via flattened [1024*8, 128] view
        m["p2c_idx"] = _idx_tile(np.arange(1024, dtype=np.int64) * 8 + k)
        m["S2T"] = S2T
        m["S2l2T"] = S2l2T
        m["P_l2"] = P_l2
        m["S3T"] = S3T
        for t in range(3):
            bw = np.zeros((96, 128), np.float32)
            for j in range(4):
                bg = 4 * k + j
                bw[3 * bg:3 * bg + 3, 32 * j:32 * j + 32] = Wm1[t]
            m[f"bigw0_{t}"] = bw.astype(H16)
        for lev in range(4):
            for t in range(3):
                m[f"bigw{lev + 1}_{t}"] = np.kron(eye4, Wm[lev][t]).astype(H16)
        for lev, nm in ((1, "b1"), (2, "b2"), (3, "b3"), (4, "b4"), (5, "b5")):
            m[f"bias{lev}"] = np.tile(d[nm].astype(np.float32), 4).reshape(128, 1)
        for li in (6, 7, 8):
            W = d[f"W{li}"].astype(np.float32)[:, 512 * k:512 * k + 512]
            m[f"w{li}"] = np.ascontiguousarray(
                W.reshape(32, 128, 512).transpose(1, 0, 2).reshape(128, 32 * 512)
            ).astype(H16)
            m[f"g{li}"] = np.ascontiguousarray(
                d[f"g{li}"].astype(np.float32)[512 * k:512 * k + 512].reshape(4, 128).T)
            m[f"be{li}"] = np.ascontiguousarray(
                d[f"be{li}"].astype(np.float32)[512 * k:512 * k + 512].reshape(4, 128).T)
        W9 = d["W9"].astype(np.float32)[512 * k:512 * k + 512]  # [512, 128]
        m["w9"] = np.ascontiguousarray(
            W9.reshape(4, 128, 128).transpose(1, 0, 2).reshape(128, 512)).astype(H16)
        per_core.append(m)

    meta = {"p1": ncw_p1, "p2": ncw_p2, "q1": ncw_q1, "q2": ncw_q2}
    return per_core, meta


# ---------------------------------------------------------------- device program
def _build_nc(meta, shapes):
    nc = bacc.Bacc("TRN2", target_bir_lowering=False, debug=False, num_devices=NCORES)
    ein = {}
    for name, arr in shapes.items():
        dt = {np.dtype(np.int16): I16, np.dtype(H16): F16,
              np.dtype(np.float32): F32}[arr.dtype]
        ein[name] = nc.dram_tensor(name, list(arr.shape), dt, kind="ExternalInput")
    out_mu = nc.dram_tensor("mu", [128, 32], F32, kind="ExternalOutput")

    tx1_c = [nc.dram_tensor(f"tx1_c{i}", [512, 128], F16) for i in range(4)]
    tx1_all = nc.dram_tensor("tx1_all", [N0, 128], F16)   # 4 AG chunks of 4096
    lv0_loc = nc.dram_tensor("lv0_loc", [192, 512], F16)  # p2t | p1l0T
    lv0_all = nc.dram_tensor("lv0_all", [1536, 512], F16)
    z1_loc = nc.dram_tensor("z1_loc", [N1, 128], F16)
    z1_c = [nc.dram_tensor(f"z1_c{i}", [1024, 128], F16) for i in range(4)]
    z1_slab = nc.dram_tensor("z1_slab", [8 * N1, 128], F16)  # 4 AG chunks of 8192
    z1_all = nc.dram_tensor("z1_all", [N1, 1024], F16)       # node-order wide
    t1_c = [nc.dram_tensor(f"t1_c{i}", [128, 1024], F16) for i in range(4)]
    t1_all = nc.dram_tensor("t1_all", [N1, 1024], F16)       # 4 AG chunks of 1024
    p2_loc = nc.dram_tensor("p2_loc", [128, 1024], F16)
    p2_all = nc.dram_tensor("p2_all", [1024, 1024], F16)     # true l1-pos order
    x_loc = nc.dram_tensor("x_loc", [4, 4096], F16)
    x_all = nc.dram_tensor("x_all", [32, 4096], F16)
    h6_loc = nc.dram_tensor("h6_loc", [128, 128], F16)
    h6_all = nc.dram_tensor("h6_all", [1024, 128], F16)
    h7_loc = nc.dram_tensor("h7_loc", [128, 128], F16)
    h7_all = nc.dram_tensor("h7_all", [1024, 128], F16)
    p9_loc = nc.dram_tensor("p9_loc", [32, 128], F32)
    p9_all = nc.dram_tensor("p9_all", [256, 128], F32)

    def ag(loc_ap, all_ap):
        nc.gpsimd.collective_compute(
            "AllGather", ALU.bypass, replica_groups=RG,
            ins=[loc_ap.opt()], outs=[all_ap.opt()])

    with tile.TileContext(nc) as tc:
        with (
            tc.tile_pool(name="const", bufs=1) as cpool,
            tc.tile_pool(name="big", bufs=1) as bigpool,
            tc.tile_pool(name="work", bufs=3) as wpool,
            tc.tile_pool(name="wload", bufs=2) as wlpool,
            tc.tile_pool(name="psA", bufs=3, space="PSUM") as ppool,
            tc.tile_pool(name="psT", bufs=2, space="PSUM") as tpool,
            tc.tile_pool(name="psB", bufs=1, space="PSUM") as apool,
        ):
            ident = cpool.tile([128, 128], F16, tag="identbf", name="identbf")
            nc.sync.dma_start(out=ident[:], in_=ein["identbf"][:, :])
            eps_t = cpool.tile([128, 1], F32, tag="epsv", name="epsv")
            nc.sync.dma_start(out=eps_t[:], in_=ein["epsv"][:, :])

            def load_const(name, dt=F16):
                t = cpool.tile(list(shapes[name].shape), dt, tag=name)
                nc.sync.dma_start(out=t[:], in_=ein[name][:, :])
                return t

            GRP = 16

            def grp_gather(idx_sb, g0, gc, gather_src, width):
                src_ap = gather_src if isinstance(gather_src, bass.AP) \
                    else gather_src[:, :]
                zb = wpool.tile([128, 4 * 1024], F16, tag="zb", name="zb", bufs=3)
                nc.gpsimd.dma_gather(
                    out_ap=zb[:, :gc * width].rearrange("p (c e) -> p c e", e=width),
                    in_ap=src_ap,
                    idxs_ap=idx_sb[:, g0 * 8:(g0 + gc) * 8],
                    num_idxs=gc * 128, num_idxs_reg=gc * 128, elem_size=width,
                    single_packet=False)
                return zb

            def grp_load(pref, g0, gc, width, tag="selg"):
                sl = wpool.tile([128, GRP * 128], F16, tag=tag, name=tag, bufs=3)
                nc.sync.dma_start(out=sl[:, :gc * width],
                                  in_=ein[pref][:, g0 * width:(g0 + gc) * width])
                return sl

            def transp(src_ap, dst_ap):
                p, f = src_ap.shape
                ps = tpool.tile([128, 128], F16, tag="tp", name="tp")
                nc.tensor.transpose(out=ps[:f, :p], in_=src_ap, identity=ident[:p, :p])
                nc.scalar.activation(out=dst_ap, in_=ps[:f, :p], func=AF.Copy)

            def load_idx(pref, C):
                s = cpool.tile([128, C * 8], I16, tag=pref + "s", name=pref + "s")
                nc.sync.dma_start(out=s[:], in_=ein[pref + "_srcw"][:, :])
                return s

            def einsum_win(bigw, taps, Din, width, out_ap, func, bias_ap):
                ps = ppool.tile([128, 512], F32, tag="ps", name="ps")
                for t in range(3):
                    nc.tensor.matmul(out=ps[:, :width], lhsT=bigw[t][:Din, :],
                                     rhs=taps[t], start=(t == 0), stop=(t == 2))
                f2 = AF.Identity if func == AF.Copy else func
                nc.scalar.activation(out=out_ap, in_=ps[:, :width], func=f2, bias=bias_ap)

            # ================= LEVEL 0 prop1: Tx1 = S0 @ X0 (dest-sharded) ===
            # X0 rows come pregathered from host (X0g_p1) -> no indirect DMA.
            with nc.named_scope("l0_prop1"):
                C1 = sum(meta["p1"])
                cur1 = {"g0": -1}

                def p1_group(cc):
                    g0 = (cc // GRP) * GRP
                    if g0 != cur1["g0"]:
                        gc = min(GRP, C1 - g0)
                        cur1["g0"] = g0
                        cur1["zb"] = grp_load("X0g_p1", g0, gc, 128, tag="zb")
                        cur1["sl"] = grp_load("p1_sel", g0, gc, 128)
                    return cur1, (cc - cur1["g0"]) * 128

                base = 0
                for wi, nch in enumerate(meta["p1"]):
                    ps = ppool.tile([128, 512], F32, tag="ps", name="ps")
                    for c in range(nch):
                        g, lo = p1_group(base + c)
                        nc.tensor.matmul(out=ps[:, :96], lhsT=g["sl"][:, lo:lo + 128],
                                         rhs=g["zb"][:, lo:lo + 96],
                                         start=(c == 0), stop=(c == nch - 1))
                    t = wpool.tile([128, 96], F16, tag="ev", name="ev", bufs=6)
                    nc.scalar.activation(out=t[:], in_=ps[:, :96], func=AF.Copy)
                    nc.sync.dma_start(
                        out=tx1_c[wi // 4][(wi % 4) * 128:(wi % 4 + 1) * 128, :96],
                        in_=t[:])
                    base += nch
                    # chunked AllGather: after windows {3,7,11,15}
                    if wi % 4 == 3:
                        cid = wi // 4
                        with nc.named_scope(f"ag1_{cid}"):
                            ag(tx1_c[cid].ap(),
                               tx1_all.ap()[4096 * cid:4096 * (cid + 1)])

            # ====== LEVEL 0 prop2 (p2t = S0[l0] @ Tx1) + p1l0 (S0[l0] @ X0) ==
            with nc.named_scope("l0_prop2"):
                C2 = sum(meta["p2"])
                s2c = load_idx("p2", C2)
                cur2 = {"g0": -1}

                def p2_group(cc):
                    g0 = (cc // GRP) * GRP
                    if g0 != cur2["g0"]:
                        gc = min(GRP, C2 - g0)
                        cur2["g0"] = g0
                        cur2["zbt"] = grp_gather(s2c, g0, gc, tx1_all, 128)
                        cur2["zbx"] = grp_load("X0g_p2", g0, gc, 128, tag="zbx")
                        cur2["sl"] = grp_load("p2_sel", g0, gc, 128)
                    return cur2, (cc - cur2["g0"]) * 128

                p2t_sb = bigpool.tile([96, 512], F16, tag="p2t_sb", name="p2t_sb")
                p1l0_sb = bigpool.tile([96, 512], F16, tag="p1l0_sb", name="p1l0_sb")
                base = 0
                for wi, nch in enumerate(meta["p2"]):
                    psa = ppool.tile([128, 512], F32, tag="ps", name="ps")
                    psb = ppool.tile([128, 512], F32, tag="ps", name="ps")
                    for c in range(nch):
                        g, lo = p2_group(base + c)
                        nc.tensor.matmul(out=psa[:96, :128], lhsT=g["zbt"][:, lo:lo + 96],
                                         rhs=g["sl"][:, lo:lo + 128],
                                         start=(c == 0), stop=(c == nch - 1))
                        nc.tensor.matmul(out=psb[:96, :128], lhsT=g["zbx"][:, lo:lo + 96],
                                         rhs=g["sl"][:, lo:lo + 128],
                                         start=(c == 0), stop=(c == nch - 1))
                    nc.scalar.activation(out=p2t_sb[:, wi * 128:(wi + 1) * 128],
                                         in_=psa[:96, :128], func=AF.Copy)
                    nc.scalar.activation(out=p1l0_sb[:, wi * 128:(wi + 1) * 128],
                                         in_=psb[:96, :128], func=AF.Copy)
                    base += nch
                nc.sync.dma_start(out=lv0_loc[0:96, :], in_=p2t_sb[:])
                nc.sync.dma_start(out=lv0_loc[96:192, :], in_=p1l0_sb[:])
            with nc.named_scope("ag2"):
                ag(lv0_loc.ap(), lv0_all.ap())

            # ================= LEVEL 0 einsum -> z1 ==========================
            with nc.named_scope("l0_einsum"):
                bw0 = [load_const(f"bigw0_{t}") for t in range(3)]
                bias1 = load_const("bias1", F32)
                for w in range(8):
                    g0w = wpool.tile([96, 512], F16, tag="g0w", name="g0w")
                    nc.sync.dma_start(out=g0w[:], in_=ein["X0l0T"][:, 512 * w:512 * (w + 1)])
                    g1w = wpool.tile([96, 512], F16, tag="g1w", name="g1w")
                    nc.sync.dma_start(out=g1w[:], in_=lv0_all[192 * w + 96:192 * w + 192, :])
                    p2w = wpool.tile([96, 512], F16, tag="p2w", name="p2w")
                    nc.sync.dma_start(out=p2w[:], in_=lv0_all[192 * w:192 * w + 96, :])
                    z1Tw = wpool.tile([128, 512], F16, tag="z1Tw", name="z1Tw")
                    einsum_win(bw0, [g0w[:], g1w[:], p2w[:]], 96, 512,
                               z1Tw[:], AF.Copy, bias1[:, 0:1])
                    for c in range(4):
                        t = wpool.tile([128, 128], F16, tag="z1nc", name="z1nc")
                        transp(z1Tw[:, c * 128:(c + 1) * 128], t[:])
                        r = w * 512 + c * 128
                        nc.sync.dma_start(out=z1_loc[r:r + 128, :], in_=t[:])
                        nc.sync.dma_start(
                            out=z1_c[w // 2][(r % 1024):(r % 1024) + 128, :], in_=t[:])
                    if w % 2 == 1:
                        cid = w // 2
                        with nc.named_scope(f"agz1_{cid}"):
                            ag(z1_c[cid].ap(),
                               z1_slab.ap()[8192 * cid:8192 * (cid + 1)])
                        # reshuffle slab [8,1024,128] -> z1_all[1024c:, r*128:...]
                        nc.sync.dma_start(
                            out=z1_all.ap()[1024 * cid:1024 * (cid + 1)]
                                .rearrange("n (r d) -> r n d", d=128),
                            in_=z1_slab.ap()[8192 * cid:8192 * (cid + 1)]
                                .rearrange("(r n) d -> r n d", n=1024))

            # ===== LEVEL 1 prop1: t1 = S1 @ z1 (dest-sharded, 1024-wide) =====
            with nc.named_scope("l1_prop1"):
                Cq1 = sum(meta["q1"])
                sq1 = load_idx("q1", Cq1)
                curq = {"g0": -1}

                def q1_group(cc):
                    g0 = (cc // 4) * 4
                    if g0 != curq["g0"]:
                        gc = min(4, Cq1 - g0)
                        curq["g0"] = g0
                        curq["zb"] = grp_gather(sq1, g0, gc, z1_all, 1024)
                        curq["sl"] = grp_load("q1_sel", g0, gc, 128)
                    return curq, (cc - curq["g0"])

                base = 0
                for wi, nch in enumerate(meta["q1"]):
                    psa = ppool.tile([128, 512], F32, tag="ps", name="ps")
                    psb = ppool.tile([128, 512], F32, tag="ps", name="ps")
                    for c in range(nch):
                        g, ci = q1_group(base + c)
                        lo = ci * 1024
                        sl_lo = ci * 128
                        nc.tensor.matmul(out=psa[:, :512], lhsT=g["sl"][:, sl_lo:sl_lo + 128],
                                         rhs=g["zb"][:, lo:lo + 512],
                                         start=(c == 0), stop=(c == nch - 1))
                        nc.tensor.matmul(out=psb[:, :512], lhsT=g["sl"][:, sl_lo:sl_lo + 128],
                                         rhs=g["zb"][:, lo + 512:lo + 1024],
                                         start=(c == 0), stop=(c == nch - 1))
                    t = wpool.tile([128, 1024], F16, tag="evw", name="evw", bufs=4)
                    nc.scalar.activation(out=t[:, :512], in_=psa[:, :512], func=AF.Copy)
                    nc.scalar.activation(out=t[:, 512:], in_=psb[:, :512], func=AF.Copy)
                    nc.sync.dma_start(out=t1_c[wi][:, :], in_=t[:])
                    base += nch
                    with nc.named_scope(f"agt1_{wi}"):
                        ag(t1_c[wi].ap(),
                           t1_all.ap()[1024 * wi:1024 * (wi + 1)])

            # ===== LEVEL 1 prop2: p2 = S1[l1] @ t1 (dest-sharded) ============
            with nc.named_scope("l1_prop2"):
                Cq2 = sum(meta["q2"])
                sq2 = load_idx("q2", Cq2)
                p2sb = bigpool.tile([128, 1024], F16, tag="p2sb", name="p2sb")
                psa = ppool.tile([128, 512], F32, tag="ps", name="ps")
                psb = ppool.tile([128, 512], F32, tag="ps", name="ps")
                nch = meta["q2"][0]
                zbw = slg = None
                for c in range(nch):
                    g0 = (c // 4) * 4
                    if c % 4 == 0:
                        gc = min(4, Cq2 - g0)
                        zbw = grp_gather(sq2, g0, gc, t1_all, 1024)
                        slg = grp_load("q2_sel", g0, gc, 128)
                    lo = (c - g0) * 1024
                    sl_lo = (c - g0) * 128
                    nc.tensor.matmul(out=psa[:, :512], lhsT=slg[:, sl_lo:sl_lo + 128],
                                     rhs=zbw[:, lo:lo + 512],
                                     start=(c == 0), stop=(c == nch - 1))
                    nc.tensor.matmul(out=psb[:, :512], lhsT=slg[:, sl_lo:sl_lo + 128],
                                     rhs=zbw[:, lo + 512:lo + 1024],
                                     start=(c == 0), stop=(c == nch - 1))
                nc.scalar.activation(out=p2sb[:, :512], in_=psa[:, :512], func=AF.Copy)
                nc.scalar.activation(out=p2sb[:, 512:], in_=psb[:, :512], func=AF.Copy)
                nc.sync.dma_start(out=p2_loc[:, :], in_=p2sb[:])
            with nc.named_scope("agp2"):
                ag(p2_loc.ap(), p2_all.ap())

            # ================= LEVEL 1 einsum -> z2 (batch-sharded) ==========
            z2n = bigpool.tile([128, 8 * 128], F16, tag="z2n", name="z2n")
            with nc.named_scope("l1_einsum"):
                l1z = cpool.tile([128, 8 * 8], I16, tag="l1z", name="l1z")
                nc.sync.dma_start(out=l1z[:], in_=ein["l1z_idx"][:, :])
                l1t = cpool.tile([128, 8 * 8], I16, tag="l1t", name="l1t")
                nc.sync.dma_start(out=l1t[:], in_=ein["l1t_idx"][:, :])
                p2c = cpool.tile([128, 8 * 8], I16, tag="p2c", name="p2c")
                nc.sync.dma_start(out=p2c[:], in_=ein["p2c_idx"][:, :])
                # z1[l1]: gather own z1_loc rows (own 128 featcols)
                z1l1T = bigpool.tile([128, 1024], F16, tag="z1l1T", name="z1l1T")
                zbz = grp_gather(l1z, 0, 8, z1_loc, 128)
                for ci in range(8):
                    transp(zbz[:, ci * 128:(ci + 1) * 128],
                           z1l1T[:, ci * 128:(ci + 1) * 128])
                # t1[l1]: per-core sub-row gather from flattened t1_all view
                t1_flat = t1_all.ap().rearrange("n (s d) -> (n s) d", d=128)
                t1l1T = bigpool.tile([128, 1024], F16, tag="t1l1T", name="t1l1T")
                zbt = grp_gather(l1t, 0, 8, t1_flat, 128)
                for ci in range(8):
                    transp(zbt[:, ci * 128:(ci + 1) * 128],
                           t1l1T[:, ci * 128:(ci + 1) * 128])
                # p2[l1] = p2_all rows (already l1-restricted): own column slice
                p2_flat = p2_all.ap().rearrange("n (s d) -> (n s) d", d=128)
                p2l1T = bigpool.tile([128, 1024], F16, tag="p2l1T", name="p2l1T")
                zbp = grp_gather(p2c, 0, 8, p2_flat, 128)
                for ci in range(8):
                    transp(zbp[:, ci * 128:(ci + 1) * 128],
                           p2l1T[:, ci * 128:(ci + 1) * 128])
                bw1 = [load_const(f"bigw1_{t}") for t in range(3)]
                bias2 = load_const("bias2", F32)
                z2T = bigpool.tile([128, 1024], F16, tag="z2T", name="z2T")
                for w in range(2):
                    einsum_win(bw1, [z1l1T[:, 512 * w:512 * (w + 1)],
                                     t1l1T[:, 512 * w:512 * (w + 1)],
                                     p2l1T[:, 512 * w:512 * (w + 1)]],
                               128, 512, z2T[:, 512 * w:512 * (w + 1)], AF.Tanh, bias2[:, 0:1])
                for c in range(8):
                    transp(z2T[:, c * 128:(c + 1) * 128], z2n[:, c * 128:(c + 1) * 128])

            # ================= LEVEL 2 (dense) =================
            with nc.named_scope("l2"):
                t1_l2 = bigpool.tile([128, 8 * 128], F16, tag="t1_l2", name="t1_l2")
                for half in range(2):
                    s2t = wlpool.tile([128, 4096], F16, tag="wld", name="wld")
                    nc.sync.dma_start(out=s2t[:], in_=ein["S2T"][:, 4096 * half:4096 * (half + 1)])
                    for dc in range(8):
                        ps = ppool.tile([128, 512], F32, tag="ps", name="ps")
                        for kk in range(4):
                            kc = half * 4 + kk
                            nc.tensor.matmul(
                                out=ps[:, :128],
                                lhsT=s2t[:, kk * 1024 + dc * 128: kk * 1024 + dc * 128 + 128],
                                rhs=z2n[:, kc * 128:(kc + 1) * 128],
                                start=(kk == 0), stop=(kk == 3))
                        if half == 0:
                            nc.scalar.activation(out=t1_l2[:, dc * 128:(dc + 1) * 128],
                                                 in_=ps[:, :128], func=AF.Copy)
                        else:
                            nc.vector.tensor_add(t1_l2[:, dc * 128:(dc + 1) * 128],
                                                 t1_l2[:, dc * 128:(dc + 1) * 128],
                                                 ps[:, :128])
                s2l2 = cpool.tile([128, 1024], F16, tag="s2l2", name="s2l2")
                nc.sync.dma_start(out=s2l2[:], in_=ein["S2l2T"][:, :])
                ps = ppool.tile([128, 512], F32, tag="ps", name="ps")
                for kc in range(8):
                    nc.tensor.matmul(out=ps[:, :128], lhsT=s2l2[:, kc * 128:(kc + 1) * 128],
                                     rhs=t1_l2[:, kc * 128:(kc + 1) * 128],
                                     start=(kc == 0), stop=(kc == 7))
                p2n_l2 = wpool.tile([128, 128], F16, tag="p2n_l2", name="p2n_l2")
                nc.scalar.activation(out=p2n_l2[:], in_=ps[:, :128], func=AF.Copy)
                pl2 = cpool.tile([128, 1024], F16, tag="pl2", name="pl2")
                nc.sync.dma_start(out=pl2[:], in_=ein["P_l2"][:, :])
                z2l2T = wpool.tile([128, 128], F16, tag="z2l2T", name="z2l2T")
                psg = ppool.tile([128, 512], F32, tag="ps", name="ps")
                for kc in range(8):
                    nc.tensor.matmul(out=psg[:, :128], lhsT=z2n[:, kc * 128:(kc + 1) * 128],
                                     rhs=pl2[:, kc * 128:(kc + 1) * 128],
                                     start=(kc == 0), stop=(kc == 7))
                nc.scalar.activation(out=z2l2T[:], in_=psg[:, :128], func=AF.Copy)
                t1l2T = wpool.tile([128, 128], F16, tag="t1l2T", name="t1l2T")
                psg2 = ppool.tile([128, 512], F32, tag="ps", name="ps")
                for kc in range(8):
                    nc.tensor.matmul(out=psg2[:, :128], lhsT=t1_l2[:, kc * 128:(kc + 1) * 128],
                                     rhs=pl2[:, kc * 128:(kc + 1) * 128],
                                     start=(kc == 0), stop=(kc == 7))
                nc.scalar.activation(out=t1l2T[:], in_=psg2[:, :128], func=AF.Copy)
                p2l2T = wpool.tile([128, 128], F16, tag="p2l2T", name="p2l2T")
                transp(p2n_l2[:], p2l2T[:])
                bw2 = [load_const(f"bigw2_{t}") for t in range(3)]
                bias3 = load_const("bias3", F32)
                z3T = wpool.tile([128, 128], F16, tag="z3T", name="z3T")
                einsum_win(bw2, [z2l2T[:], t1l2T[:], p2l2T[:]], 128, 128,
                           z3T[:], AF.Tanh, bias3[:, 0:1])
                z3n = wpool.tile([128, 128], F16, tag="z3n", name="z3n")
                transp(z3T[:], z3n[:])

            # ================= LEVEL 3 =================
            with nc.named_scope("l3"):
                s3t = cpool.tile([128, 128], F16, tag="s3t", name="s3t")
                nc.sync.dma_start(out=s3t[:], in_=ein["S3T"][:, :])
                bias4 = load_const("bias4", F32)
                bias5 = load_const("bias5", F32)

                def conv_l3(zn, zT, bw_pref, bias_t, func, keep):
                    t1T = wpool.tile([128, 128], F16, tag=keep + "t1T", name=keep + "t1T")
                    ps = ppool.tile([128, 512], F32, tag="ps", name="ps")
                    nc.tensor.matmul(out=ps[:, :128], lhsT=zn, rhs=s3t[:], start=True, stop=True)
                    nc.scalar.activation(out=t1T[:], in_=ps[:, :128], func=AF.Copy)
                    t1n_ = wpool.tile([128, 128], F16, tag=keep + "t1n", name=keep + "t1n")
                    transp(t1T[:], t1n_[:])
                    p2T_ = wpool.tile([128, 128], F16, tag=keep + "p2T", name=keep + "p2T")
                    ps2 = ppool.tile([128, 512], F32, tag="ps", name="ps")
                    nc.tensor.matmul(out=ps2[:, :128], lhsT=t1n_[:], rhs=s3t[:], start=True, stop=True)
                    nc.scalar.activation(out=p2T_[:], in_=ps2[:, :128], func=AF.Copy)
                    bw = [load_const(f"{bw_pref}_{t}") for t in range(3)]
                    outT = wpool.tile([128, 128], F16, tag=keep + "oT", name=keep + "oT")
                    einsum_win(bw, [zT, t1T[:], p2T_[:]], 128, 128, outT[:], func, bias_t[:, 0:1])
                    outn = wpool.tile([128, 128], F16, tag=keep + "on", name=keep + "on")
                    transp(outT[:], outn[:])
                    return outn, outT

                z4n, z4T = conv_l3(z3n[:], z3T[:], "bigw3", bias4, AF.Tanh, "c4")
                o5n, o5T = conv_l3(z4n[:], z4T[:], "bigw4", bias5, AF.Copy, "c5")

            # ================= MLP input assembly =================
            with nc.named_scope("mlp_in"):
                nc.sync.dma_start(
                    out=x_loc.ap().rearrange("b (n c) -> n b c", c=32),
                    in_=o5n[:].rearrange("n (b c) -> n b c", c=32))
                ag(x_loc.ap(), x_all.ap())
                xT_sb = bigpool.tile([32, 4096], F16, tag="xT_sb", name="xT_sb")
                nc.sync.dma_start(out=xT_sb[:], in_=x_all[:, :])
                act6 = bigpool.tile([128, 1024], F16, tag="act6", name="act6")
                for i in range(32):
                    transp(xT_sb[:, 128 * i:128 * (i + 1)], act6[:, 32 * i:32 * i + 32])

            # ================= MLP =================
            def mlp_layer(li, act_sb, out_sb):
                g_t = load_const(f"g{li}", F32)
                be_t = load_const(f"be{li}", F32)
                wt = wlpool.tile([128, 32 * 512], F16, tag="wld", name="wld")
                nc.sync.dma_start(out=wt[:, :8192], in_=ein[f"w{li}"][:, :8192])
                nc.sync.dma_start(out=wt[:, 8192:], in_=ein[f"w{li}"][:, 8192:])
                acc = apool.tile([128, 512], F32, tag="acc", name="acc")
                for k2 in range(32):
                    nc.tensor.matmul(out=acc[:32, :], lhsT=act_sb[:, 32 * k2:32 * k2 + 32],
                                     rhs=wt[:, 512 * k2:512 * (k2 + 1)],
                                     start=(k2 == 0), stop=(k2 == 31))
                hb = wpool.tile([32, 512], F16, tag="hb", name="hb")
                nc.scalar.activation(out=hb[:], in_=acc[:32, :], func=AF.Copy)
                for c in range(4):
                    hc = wpool.tile([128, 32], F16, tag="hc", name="hc")
                    transp(hb[:, 128 * c:128 * (c + 1)], hc[:])
                    s1 = wpool.tile([128, 1], F32, tag="b_s1", name="b_s1")
                    nc.vector.tensor_reduce(out=s1[:], in_=hc[:], axis=AX.X, op=ALU.add)
                    mu_ = wpool.tile([128, 1], F32, tag="b_mu", name="b_mu")
                    nc.vector.tensor_scalar_mul(mu_[:], s1[:], 1.0 / 32.0)
                    sq = wpool.tile([128, 32], F32, tag="b_sq", name="b_sq")
                    nc.vector.tensor_mul(sq[:], hc[:], hc[:])
                    s2_ = wpool.tile([128, 1], F32, tag="b_s2", name="b_s2")
                    nc.vector.tensor_reduce(out=s2_[:], in_=sq[:], axis=AX.X, op=ALU.add)
                    var = wpool.tile([128, 1], F32, tag="b_var", name="b_var")
                    nc.vector.scalar_tensor_tensor(out=var[:], in0=mu_[:], scalar=-1.0,
                                                   in1=mu_[:], op0=ALU.mult, op1=ALU.mult)
                    nc.vector.scalar_tensor_tensor(out=var[:], in0=s2_[:], scalar=1.0 / 32.0,
                                                   in1=var[:], op0=ALU.mult, op1=ALU.add)
                    sd = wpool.tile([128, 1], F32, tag="b_sd", name="b_sd")
                    nc.scalar.activation(out=sd[:], in_=var[:], func=AF.Sqrt, bias=eps_t[:, 0:1])
                    rs = wpool.tile([128, 1], F32, tag="b_rs", name="b_rs")
                    nc.vector.reciprocal(rs[:], sd[:])
                    a_ = wpool.tile([128, 1], F32, tag="b_a", name="b_a")
                    nc.vector.tensor_mul(a_[:], rs[:], g_t[:, c:c + 1])
                    sh = wpool.tile([128, 1], F32, tag="b_sh", name="b_sh")
                    nc.vector.scalar_tensor_tensor(out=sh[:], in0=mu_[:], scalar=-1.0,
                                                   in1=a_[:], op0=ALU.mult, op1=ALU.mult)
                    nc.vector.tensor_add(sh[:], sh[:], be_t[:, c:c + 1])
                    nc.scalar.activation(out=out_sb[:, 32 * c:32 * c + 32], in_=hc[:],
                                         func=AF.Relu, scale=a_[:, 0:1], bias=sh[:, 0:1])

            with nc.named_scope("mlp6"):
                h6 = bigpool.tile([128, 128], F16, tag="h6sb", name="h6sb")
                mlp_layer(6, act6, h6)
                nc.sync.dma_start(out=h6_loc.ap(), in_=h6[:])
                ag(h6_loc.ap(), h6_all.ap())
            with nc.named_scope("mlp7"):
                act7 = bigpool.tile([128, 1024], F16, tag="act7", name="act7")
                for r in range(8):
                    nc.sync.dma_start(out=act7[:, 128 * r:128 * (r + 1)],
                                      in_=h6_all[128 * r:128 * (r + 1), :])
                h7 = bigpool.tile([128, 128], F16, tag="h7sb", name="h7sb")
                mlp_layer(7, act7, h7)
                nc.sync.dma_start(out=h7_loc.ap(), in_=h7[:])
                ag(h7_loc.ap(), h7_all.ap())
            with nc.named_scope("mlp8"):
                act8 = bigpool.tile([128, 1024], F16, tag="act8", name="act8")
                for r in range(8):
                    nc.sync.dma_start(out=act8[:, 128 * r:128 * (r + 1)],
                                      in_=h7_all[128 * r:128 * (r + 1), :])
                h8 = bigpool.tile([128, 128], F16, tag="h8sb", name="h8sb")
                mlp_layer(8, act8, h8)

            with nc.named_scope("mlp9"):
                w9t = cpool.tile([128, 512], F16, tag="w9t", name="w9t")
                nc.sync.dma_start(out=w9t[:], in_=ein["w9"][:, :])
                acc9 = apool.tile([128, 512], F32, tag="acc", name="acc9")
                for c in range(4):
                    nc.tensor.matmul(out=acc9[:32, :128], lhsT=h8[:, 32 * c:32 * c + 32],
                                     rhs=w9t[:, 128 * c:128 * (c + 1)],
                                     start=(c == 0), stop=(c == 3))
                p9sb = wpool.tile([32, 128], F32, tag="p9sb", name="p9sb")
                nc.scalar.activation(out=p9sb[:], in_=acc9[:32, :128], func=AF.Copy)
                nc.sync.dma_start(out=p9_loc.ap(), in_=p9sb[:])
                ag(p9_loc.ap(), p9_all.ap())
                tot = wpool.tile([32, 128], F32, tag="f_tot", name="f_tot")
                nc.sync.dma_start(out=tot[:], in_=p9_all[0:32, :])
                for k in range(1, 8):
                    pk = wpool.tile([32, 128], F32, tag="f_pk", name="f_pk")
                    nc.sync.dma_start(out=pk[:], in_=p9_all[32 * k:32 * (k + 1), :])
                    nc.vector.tensor_add(tot[:], tot[:], pk[:])
                totT = wpool.tile([128, 32], F32, tag="f_totT", name="f_totT")
                pst = ppool.tile([128, 512], F32, tag="ps", name="pst")
                identf = cpool.tile([32, 32], F32, tag="identf", name="identf")
                nc.scalar.activation(out=identf[:], in_=ident[:32, :32], func=AF.Copy)
                nc.tensor.transpose(out=pst[:128, :32], in_=tot[:], identity=identf[:])
                nc.scalar.activation(out=totT[:], in_=pst[:128, :32], func=AF.Copy)
                s1 = wpool.tile([128, 1], F32, tag="f_s1", name="f_s1")
                nc.vector.tensor_reduce(out=s1[:], in_=totT[:], axis=AX.X, op=ALU.add)
                mu_ = wpool.tile([128, 1], F32, tag="f_mu", name="f_mu")
                nc.vector.tensor_scalar_mul(mu_[:], s1[:], 1.0 / 32.0)
                sq = wpool.tile([128, 32], F32, tag="f_sq", name="f_sq")
                nc.vector.tensor_mul(sq[:], totT[:], totT[:])
                s2_ = wpool.tile([128, 1], F32, tag="f_s2", name="f_s2")
                nc.vector.tensor_reduce(out=s2_[:], in_=sq[:], axis=AX.X, op=ALU.add)
                var = wpool.tile([128, 1], F32, tag="f_var", name="f_var")
                nc.vector.scalar_tensor_tensor(out=var[:], in0=mu_[:], scalar=-1.0,
                                               in1=mu_[:], op0=ALU.mult, op1=ALU.mult)
                nc.vector.scalar_tensor_tensor(out=var[:], in0=s2_[:], scalar=1.0 / 32.0,
                                               in1=var[:], op0=ALU.mult, op1=ALU.add)
                sdf = wpool.tile([128, 1], F32, tag="f_sd", name="f_sd")
                nc.scalar.activation(out=sdf[:], in_=var[:], func=AF.Sqrt, bias=eps_t[:, 0:1])
                rs = wpool.tile([128, 1], F32, tag="f_rs", name="f_rs")
                nc.vector.reciprocal(rs[:], sdf[:])
                neg = wpool.tile([128, 1], F32, tag="f_neg", name="f_neg")
                nc.vector.scalar_tensor_tensor(out=neg[:], in0=mu_[:], scalar=-1.0,
                                               in1=rs[:], op0=ALU.mult, op1=ALU.mult)
                outt = wpool.tile([128, 32], F32, tag="f_out", name="f_out")
                nc.scalar.activation(out=outt[:], in_=totT[:], func=AF.Identity,
                                     scale=rs[:, 0:1], bias=neg[:, 0:1])
                nc.sync.dma_start(out=out_mu[:, :], in_=outt[:])

    nc.compile()
    return nc


# ---------------------------------------------------------------- entry point
def kernel(**inputs) -> np.ndarray:
    per_core, meta = _host_prep(inputs)
    if "prog" not in _CACHE:
        _CACHE["prog"] = _build_nc(meta, per_core[0])
    nc = _CACHE["prog"]
    res = bass_utils.run_bass_kernel_spmd(nc, per_core, core_ids=list(range(NCORES)))
    return np.ascontiguousarray(res.results[0]["mu"].T)


# revision 24
# speedup vs baseline: 1.9459x; 1.1063x over previous
"""Trainium2 Bass kernel for nn_Encoder_base (5x ChebConv GNN + pool + MLP).

Distribution over 8 NeuronCores (all matmuls fp16, fp32 PSUM):
  - level 0: the two props the einsum needs (Tx1[l0] = S0[l0]@X0 and
    p2t = S0[l0]@S0@X0) are composed on the HOST into single operators on
    the input X0 (2-hop edge expansion M0 = S0[l0]*S0). Edge-major X0 rows
    are pregathered host-side -> the props are pure streaming selection
    matmuls: zero indirect DMA, zero full-graph AllGather.
  - level 1: stacked dense operator T = [P_l1; S1[l1]; M1=S1[l1]*S1]
    (3072 x 4096) applied to z1, dest-sharded (128 l1-positions/core, all
    1024 batch-features wide); einsum is dest-sharded too. Comm: one
    chunked z1 AllGather + one small z2 AllGather.
  - levels 2-3: batch-sharded (4 batches/core), dense-S matmuls,
    block-diagonal channel mixes in feature-major layout.
  - MLP: output-feature sharded (512 cols of W6/7/8, 512 rows of W9 per
    core); activations [128k,32] are the stationary lhsT, W streams as rhs;
    BatchNorm per-feature after a PE transpose; activations AllGathered.
"""
import numpy as np
import concourse.bass as bass
import concourse.bacc as bacc
import concourse.tile as tile
from concourse import mybir, bass_utils

F32 = mybir.dt.float32
F16 = mybir.dt.float16
I16 = mybir.dt.int16
AF = mybir.ActivationFunctionType
ALU = mybir.AluOpType
AX = mybir.AxisListType
RG = [list(range(8))]
NCORES = 8
N0, N1, N2, N3 = 16384, 4096, 1024, 128
EPS = 1e-5
H16 = np.float16

_CACHE = {}


# ---------------------------------------------------------------- host prep
def _prep_prop(row, col, we, n_dest, n_shard):
    """Sorted-by-dest edges -> 128-dest windows, 128-edge chunks, padded so
    chunk counts per window match across shards (one SPMD program).
    Emits per-chunk selection matrices sel[chunk, edge_local, dst_local]."""
    window = 128
    order = np.argsort(row, kind="stable")
    row, col, we = row[order], col[order], we[order]
    per = n_dest // n_shard
    nwin = per // window
    counts = np.zeros((n_shard, nwin), np.int64)
    lists = {}
    for s in range(n_shard):
        lo = s * per
        for wi in range(nwin):
            wlo = lo + wi * window
            a = np.searchsorted(row, wlo, side="left")
            b = np.searchsorted(row, wlo + window, side="left")
            lists[(s, wi)] = (row[a:b] - wlo, col[a:b], we[a:b])
            counts[s, wi] = (b - a + 127) // 128
    ncw = np.maximum(counts.max(axis=0), 1)
    C = int(ncw.sum())
    src = np.zeros((n_shard, C, 128), np.int64)
    sel = np.zeros((n_shard, C, 128, 128), np.float32)
    for s in range(n_shard):
        base = 0
        for wi in range(nwin):
            dl, cl, wl = lists[(s, wi)]
            n = len(dl)
            k = int(ncw[wi])
            src[s, base:base + k].reshape(-1)[:n] = cl
            ch = base + np.arange(n) // 128
            ep = np.arange(n) % 128
            sel[s, ch, ep, dl] = wl
            base += k
    return [int(x) for x in ncw], src, sel


def _edge_we(e, n):
    row, col = np.asarray(e[0], np.int64), np.asarray(e[1], np.int64)
    deg = np.bincount(row, minlength=n).astype(np.float32)
    dis = np.where(deg > 0, 1.0 / np.sqrt(np.maximum(deg, 1.0)), 0.0).astype(np.float32)
    return row, col, -(dis[row] * dis[col]).astype(np.float32)


def _sub_edges(row, col, we, pool_idx):
    order = np.argsort(row, kind="stable")
    row, col, we = row[order], col[order], we[order]
    starts = np.searchsorted(row, pool_idx, side="left")
    ends = np.searchsorted(row, pool_idx, side="right")
    nr, ncl, nw = [], [], []
    for i in range(len(pool_idx)):
        s, e = starts[i], ends[i]
        if e > s:
            nr.append(np.full(e - s, i, np.int64))
            ncl.append(col[s:e])
            nw.append(we[s:e])
    return np.concatenate(nr), np.concatenate(ncl), np.concatenate(nw)


def _twohop(ri, ci, wi, row, col, we, n):
    """(i,j,w1) sub-edges composed with full edges (j->k,w2): (i,k,w1*w2)."""
    order = np.argsort(row, kind="stable")
    rs, cs, ws = row[order], col[order], we[order]
    starts = np.searchsorted(rs, np.arange(n), side="left")
    ends = np.searchsorted(rs, np.arange(n), side="right")
    cnt = (ends - starts)[ci]
    I = np.repeat(ri, cnt)
    W1 = np.repeat(wi, cnt)
    base = np.repeat(starts[ci], cnt)
    within = np.arange(cnt.sum()) - np.repeat(np.cumsum(cnt) - cnt, cnt)
    offs = base + within
    return I, cs[offs], W1 * ws[offs]


def _dense_s(row, col, we, n, m):
    s = np.zeros((n, m), np.float32)
    np.add.at(s, (row, col), we)
    return s


def _tile_w(w, pack):
    """[K, M] -> [K//(128*pack) * 128, pack*M]: pack K-blocks side by side."""
    k, m = w.shape
    nb = k // 128
    t = w.reshape(nb // pack, pack, 128, m).transpose(0, 2, 1, 3)
    return np.ascontiguousarray(t.reshape((nb // pack) * 128, pack * m))


def _idx_tile(flat):
    """flat int idx list -> [128, len//16] int16 (16-part wrap, x8 replicas)."""
    return np.ascontiguousarray(
        np.tile(flat.astype(np.int16).reshape(-1, 16).T, (8, 1)))


def _chunk_tile(arr3):
    """[C, 128, W] -> [128, C*W] (chunk c at cols c*W..)."""
    C, _, W = arr3.shape
    return np.ascontiguousarray(
        arr3.transpose(1, 0, 2).reshape(128, C * W)).astype(H16)


def _host_prep(inputs):
    d = {k: np.asarray(v) for k, v in inputs.items()}
    x = d["x"].astype(np.float32)
    l0 = np.asarray(d["l0"], np.int64)
    l1 = np.asarray(d["l1"], np.int64)
    l2 = np.asarray(d["l2"], np.int64)

    X0 = np.ascontiguousarray(x.transpose(1, 0, 2).reshape(N0, 96))
    X0p = np.zeros((N0, 128), np.float32)
    X0p[:, :96] = X0
    X0l0T = np.ascontiguousarray(X0[l0].T)  # [96, 4096]

    # level-0 operators on X0: a = S0[l0] (tap1), m = S0[l0]@S0 (tap2)
    r0, c0, w0 = _edge_we(d["e0"], N0)
    r0s, c0s, w0s = _sub_edges(r0, c0, w0, l0)
    ncw_a, src_a, sel_a = _prep_prop(r0s, c0s, w0s, N1, NCORES)
    mI, mK, mW = _twohop(r0s, c0s, w0s, r0, c0, w0, N0)
    ncw_m, src_m, sel_m = _prep_prop(mI, mK, mW, N1, NCORES)

    # level-1 stacked operator T = [P_l1; S1[l1]; M1]
    r1, c1, w1 = _edge_we(d["e1"], N1)
    S1 = _dense_s(r1, c1, w1, N1, N1)
    r1s, c1s, w1s = _sub_edges(r1, c1, w1, l1)
    S1l1 = _dense_s(r1s, c1s, w1s, N2, N1)    # [1024, 4096]
    M1 = S1l1 @ S1                            # [1024, 4096]
    P_l1 = np.zeros((N2, N1), np.float32)
    P_l1[np.arange(N2), l1] = 1.0
    Tblocks = [P_l1, S1l1, M1]

    r2, c2, w2 = _edge_we(d["e2"], N2)
    S2 = _dense_s(r2, c2, w2, N2, N2)
    S2T = _tile_w(np.ascontiguousarray(S2.T), 8).astype(H16)       # [128, 8192]
    S2l2T = _tile_w(np.ascontiguousarray(S2[l2].T), 8).astype(H16)  # [128, 1024]
    P_l2 = np.zeros((N2, 128), np.float32)
    P_l2[l2, np.arange(128)] = 1.0
    P_l2 = _tile_w(P_l2, 8).astype(H16)                             # [128, 1024]

    r3, c3, w3 = _edge_we(d["e3"], N3)
    S3T = np.ascontiguousarray(_dense_s(r3, c3, w3, N3, N3).T).astype(H16)

    def wmod(W):
        return W[0] - W[2], W[1], 2.0 * W[2]

    Wm1 = wmod(d["Wc1"].astype(np.float32))
    Wm = [wmod(d[f"Wc{i}"].astype(np.float32)) for i in (2, 3, 4, 5)]
    eye4 = np.eye(4, dtype=np.float32)

    per_core = []
    for k in range(NCORES):
        m = {}
        m["identbf"] = np.eye(128, dtype=np.float32).astype(H16)
        m["epsv"] = np.full((128, 1), EPS, np.float32)
        m["warm"] = np.zeros((1, 8), np.float32)
        m["X0l0T"] = X0l0T.astype(H16)
        m["Xg_a"] = _chunk_tile(X0p[src_a[k]].astype(H16))
        m["sel_a"] = _chunk_tile(sel_a[k])
        m["Xg_m"] = _chunk_tile(X0p[src_m[k]].astype(H16))
        m["sel_m"] = _chunk_tile(sel_m[k])
        # stacked-T lhsT chunks: block b, k-chunk kk at cols (b*32+kk)*128
        tt = np.zeros((128, 96 * 128), np.float32)
        for b, blk in enumerate(Tblocks):
            bt = blk[128 * k:128 * (k + 1), :].T  # [4096, 128]
            for kk in range(32):
                tt[:, (b * 32 + kk) * 128:(b * 32 + kk + 1) * 128] = \
                    bt[128 * kk:128 * (kk + 1), :]
        m["Tt"] = tt.astype(H16)
        # z2n extraction: for node-chunk c, rows 1024c + 128k + j of z2T_all
        m["z2n_idx"] = _idx_tile(np.concatenate(
            [1024 * c + 128 * k + np.arange(128) for c in range(8)]))
        m["S2T"] = S2T
        m["S2l2T"] = S2l2T
        m["P_l2"] = P_l2
        m["S3T"] = S3T
        for t in range(3):
            bw = np.zeros((96, 128), np.float32)
            for j in range(4):
                bg = 4 * k + j
                bw[3 * bg:3 * bg + 3, 32 * j:32 * j + 32] = Wm1[t]
            m[f"bigw0_{t}"] = bw.astype(H16)
        for lev in range(4):
            for t in range(3):
                m[f"bigw{lev + 1}_{t}"] = np.kron(eye4, Wm[lev][t]).astype(H16)
        for lev, nm in ((1, "b1"), (2, "b2"), (3, "b3"), (4, "b4"), (5, "b5")):
            m[f"bias{lev}"] = np.tile(d[nm].astype(np.float32), 4).reshape(128, 1)
        for li in (6, 7, 8):
            W = d[f"W{li}"].astype(np.float32)[:, 512 * k:512 * k + 512]
            m[f"w{li}"] = np.ascontiguousarray(
                W.reshape(32, 128, 512).transpose(1, 0, 2).reshape(128, 32 * 512)
            ).astype(H16)
            m[f"g{li}"] = np.ascontiguousarray(
                d[f"g{li}"].astype(np.float32)[512 * k:512 * k + 512].reshape(4, 128).T)
            m[f"be{li}"] = np.ascontiguousarray(
                d[f"be{li}"].astype(np.float32)[512 * k:512 * k + 512].reshape(4, 128).T)
        W9 = d["W9"].astype(np.float32)[512 * k:512 * k + 512]  # [512, 128]
        m["w9"] = np.ascontiguousarray(
            W9.reshape(4, 128, 128).transpose(1, 0, 2).reshape(128, 512)).astype(H16)
        per_core.append(m)

    meta = {"a": ncw_a, "m": ncw_m}
    return per_core, meta


# ---------------------------------------------------------------- device program
def _build_nc(meta, shapes):
    nc = bacc.Bacc("TRN2", target_bir_lowering=False, debug=False, num_devices=NCORES)
    ein = {}
    for name, arr in shapes.items():
        dt = {np.dtype(np.int16): I16, np.dtype(H16): F16,
              np.dtype(np.float32): F32}[arr.dtype]
        ein[name] = nc.dram_tensor(name, list(arr.shape), dt, kind="ExternalInput")
    out_mu = nc.dram_tensor("mu", [128, 32], F32, kind="ExternalOutput")

    warm_all = nc.dram_tensor("warm_all", [8, 8], F32)
    warm_loc = nc.dram_tensor("warm_loc", [1, 8], F32)
    lv0_loc = nc.dram_tensor("lv0_loc", [192, 512], F16)  # tap1 | tap2
    lv0_all = nc.dram_tensor("lv0_all", [1536, 512], F16)
    z1_c = [nc.dram_tensor(f"z1_c{i}", [1024, 128], F16) for i in range(4)]
    z1_slab = nc.dram_tensor("z1_slab", [8 * N1, 128], F16)  # 4 AG chunks of 8192
    z2T_loc = nc.dram_tensor("z2T_loc", [1024, 128], F16)
    z2T_all = nc.dram_tensor("z2T_all", [8192, 128], F16)
    x_loc = nc.dram_tensor("x_loc", [4, 4096], F16)
    x_all = nc.dram_tensor("x_all", [32, 4096], F16)
    h6_loc = nc.dram_tensor("h6_loc", [128, 128], F16)
    h6_all = nc.dram_tensor("h6_all", [1024, 128], F16)
    h7_loc = nc.dram_tensor("h7_loc", [128, 128], F16)
    h7_all = nc.dram_tensor("h7_all", [1024, 128], F16)
    p9_loc = nc.dram_tensor("p9_loc", [32, 128], F32)
    p9_all = nc.dram_tensor("p9_all", [256, 128], F32)

    def ag(loc_ap, all_ap):
        nc.gpsimd.collective_compute(
            "AllGather", ALU.bypass, replica_groups=RG,
            ins=[loc_ap.opt()], outs=[all_ap.opt()])

    with tile.TileContext(nc) as tc:
        with (
            tc.tile_pool(name="const", bufs=1) as cpool,
            tc.tile_pool(name="big", bufs=1) as bigpool,
            tc.tile_pool(name="work", bufs=3) as wpool,
            tc.tile_pool(name="wload", bufs=2) as wlpool,
            tc.tile_pool(name="psA", bufs=3, space="PSUM") as ppool,
            tc.tile_pool(name="psT", bufs=2, space="PSUM") as tpool,
            tc.tile_pool(name="psB", bufs=1, space="PSUM") as apool,
        ):
            ident = cpool.tile([128, 128], F16, tag="identbf", name="identbf")
            nc.sync.dma_start(out=ident[:], in_=ein["identbf"][:, :])
            eps_t = cpool.tile([128, 1], F32, tag="epsv", name="epsv")
            nc.sync.dma_start(out=eps_t[:], in_=ein["epsv"][:, :])

            def load_const(name, dt=F16):
                t = cpool.tile(list(shapes[name].shape), dt, tag=name)
                nc.sync.dma_start(out=t[:], in_=ein[name][:, :])
                return t

            GRP = 8

            def grp_load(pref, g0, gc, tag):
                sl = wpool.tile([128, GRP * 128], F16, tag=tag, name=tag, bufs=2)
                nc.sync.dma_start(out=sl[:, :gc * 128],
                                  in_=ein[pref][:, g0 * 128:(g0 + gc) * 128])
                return sl

            def transp(src_ap, dst_ap):
                p, f = src_ap.shape
                ps = tpool.tile([128, 128], F16, tag="tp", name="tp")
                nc.tensor.transpose(out=ps[:f, :p], in_=src_ap, identity=ident[:p, :p])
                nc.scalar.activation(out=dst_ap, in_=ps[:f, :p], func=AF.Copy)

            def einsum_win(bigw, taps, Din, width, out_ap, func, bias_ap):
                ps = ppool.tile([128, 512], F32, tag="ps", name="ps")
                for t in range(3):
                    nc.tensor.matmul(out=ps[:, :width], lhsT=bigw[t][:Din, :],
                                     rhs=taps[t], start=(t == 0), stop=(t == 2))
                f2 = AF.Identity if func == AF.Copy else func
                nc.scalar.activation(out=out_ap, in_=ps[:, :width], func=f2, bias=bias_ap)

            # warm up the CC ring while level-0 computes
            with nc.named_scope("warmup"):
                warm = wpool.tile([1, 8], F32, tag="warm", name="warm")
                nc.sync.dma_start(out=warm[:], in_=ein["warm"][:, :])
                nc.sync.dma_start(out=warm_loc.ap(), in_=warm[:])
                ag(warm_loc.ap(), warm_all.ap())

            # ====== LEVEL 0 props: tap1 = S0[l0]@X0, tap2 = (S0[l0]@S0)@X0 ===
            with nc.named_scope("l0_props"):
                Ca, Cm = sum(meta["a"]), sum(meta["m"])
                cura = {"g0": -1}
                curm = {"g0": -1}

                def get_grp(cur, cc, C, xg, sel, xtag, stag):
                    g0 = (cc // GRP) * GRP
                    if g0 != cur["g0"]:
                        gc = min(GRP, C - g0)
                        cur["g0"] = g0
                        cur["x"] = grp_load(xg, g0, gc, xtag)
                        cur["s"] = grp_load(sel, g0, gc, stag)
                    return cur, (cc - cur["g0"]) * 128

                tap1_sb = bigpool.tile([96, 512], F16, tag="tap1_sb", name="tap1_sb")
                tap2_sb = bigpool.tile([96, 512], F16, tag="tap2_sb", name="tap2_sb")
                ba, bm = 0, 0
                for wi in range(4):
                    psa = ppool.tile([128, 512], F32, tag="ps", name="ps")
                    for c in range(meta["a"][wi]):
                        g, lo = get_grp(cura, ba + c, Ca, "Xg_a", "sel_a", "xga", "sla")
                        nc.tensor.matmul(out=psa[:96, :128], lhsT=g["x"][:, lo:lo + 96],
                                         rhs=g["s"][:, lo:lo + 128],
                                         start=(c == 0), stop=(c == meta["a"][wi] - 1))
                    nc.scalar.activation(out=tap1_sb[:, wi * 128:(wi + 1) * 128],
                                         in_=psa[:96, :128], func=AF.Copy)
                    ba += meta["a"][wi]
                    psm = ppool.tile([128, 512], F32, tag="ps", name="ps")
                    for c in range(meta["m"][wi]):
                        g, lo = get_grp(curm, bm + c, Cm, "Xg_m", "sel_m", "xgm", "slm")
                        nc.tensor.matmul(out=psm[:96, :128], lhsT=g["x"][:, lo:lo + 96],
                                         rhs=g["s"][:, lo:lo + 128],
                                         start=(c == 0), stop=(c == meta["m"][wi] - 1))
                    nc.scalar.activation(out=tap2_sb[:, wi * 128:(wi + 1) * 128],
                                         in_=psm[:96, :128], func=AF.Copy)
                    bm += meta["m"][wi]
                nc.sync.dma_start(out=lv0_loc[0:96, :], in_=tap1_sb[:])
                nc.sync.dma_start(out=lv0_loc[96:192, :], in_=tap2_sb[:])
            with nc.named_scope("ag_lv0"):
                ag(lv0_loc.ap(), lv0_all.ap())

            # ================= LEVEL 0 einsum -> z1 ==========================
            with nc.named_scope("l0_einsum"):
                bw0 = [load_const(f"bigw0_{t}") for t in range(3)]
                bias1 = load_const("bias1", F32)
                for w in range(8):
                    g0w = wpool.tile([96, 512], F16, tag="g0w", name="g0w", bufs=2)
                    nc.sync.dma_start(out=g0w[:], in_=ein["X0l0T"][:, 512 * w:512 * (w + 1)])
                    g1w = wpool.tile([96, 512], F16, tag="g1w", name="g1w", bufs=2)
                    nc.sync.dma_start(out=g1w[:], in_=lv0_all[192 * w:192 * w + 96, :])
                    p2w = wpool.tile([96, 512], F16, tag="p2w", name="p2w", bufs=2)
                    nc.sync.dma_start(out=p2w[:], in_=lv0_all[192 * w + 96:192 * w + 192, :])
                    z1Tw = wpool.tile([128, 512], F16, tag="z1Tw", name="z1Tw")
                    einsum_win(bw0, [g0w[:], g1w[:], p2w[:]], 96, 512,
                               z1Tw[:], AF.Copy, bias1[:, 0:1])
                    for c in range(4):
                        t = wpool.tile([128, 128], F16, tag="z1nc", name="z1nc")
                        transp(z1Tw[:, c * 128:(c + 1) * 128], t[:])
                        r = w * 512 + c * 128
                        nc.sync.dma_start(
                            out=z1_c[w // 2][(r % 1024):(r % 1024) + 128, :], in_=t[:])
                    if w % 2 == 1:
                        cid = w // 2
                        with nc.named_scope(f"agz1_{cid}"):
                            ag(z1_c[cid].ap(),
                               z1_slab.ap()[8192 * cid:8192 * (cid + 1)])

            # ====== LEVEL 1: taps = T @ z1, dest-sharded (128 pos x 1024) ====
            with nc.named_scope("l1_T"):
                tt = cpool.tile([128, 96 * 128], F16, tag="Tt", name="Tt")
                nc.sync.dma_start(out=tt[:, :6144], in_=ein["Tt"][:, :6144])
                nc.sync.dma_start(out=tt[:, 6144:], in_=ein["Tt"][:, 6144:])
                accs = [bigpool.tile([128, 1024], F16, tag=f"accT{b}", name=f"accT{b}")
                        for b in range(3)]
                for ks in range(8):  # superchunks of 4 k-chunks (512 nodes)
                    z1sc = wpool.tile([128, 4096], F16, tag="z1sc", name="z1sc", bufs=2)
                    # z1_slab rows: 8192*quarter + 1024*core + node_in_quarter
                    for r in range(8):
                        b0 = 8192 * (ks // 2) + 1024 * r + 512 * (ks % 2)
                        for q in range(4):
                            nc.sync.dma_start(
                                out=z1sc[:, q * 1024 + r * 128:q * 1024 + (r + 1) * 128],
                                in_=z1_slab[b0 + 128 * q:b0 + 128 * (q + 1), :])
                    for b in range(3):
                        psa = ppool.tile([128, 512], F32, tag="ps", name="ps")
                        psb = ppool.tile([128, 512], F32, tag="ps", name="ps")
                        for q in range(4):
                            kk = ks * 4 + q
                            lh = tt[:, (b * 32 + kk) * 128:(b * 32 + kk + 1) * 128]
                            nc.tensor.matmul(out=psa[:, :512], lhsT=lh,
                                             rhs=z1sc[:, q * 1024:q * 1024 + 512],
                                             start=(q == 0), stop=(q == 3))
                            nc.tensor.matmul(out=psb[:, :512], lhsT=lh,
                                             rhs=z1sc[:, q * 1024 + 512:(q + 1) * 1024],
                                             start=(q == 0), stop=(q == 3))
                        if ks == 0:
                            nc.scalar.activation(out=accs[b][:, :512], in_=psa[:, :512],
                                                 func=AF.Copy)
                            nc.scalar.activation(out=accs[b][:, 512:], in_=psb[:, :512],
                                                 func=AF.Copy)
                        else:
                            nc.vector.tensor_add(accs[b][:, :512], accs[b][:, :512],
                                                 psa[:, :512])
                            nc.vector.tensor_add(accs[b][:, 512:], accs[b][:, 512:],
                                                 psb[:, :512])

            # ============ LEVEL 1 einsum (dest-sharded) -> z2T ===============
            with nc.named_scope("l1_einsum"):
                bw1 = [load_const(f"bigw1_{t}") for t in range(3)]
                bias2 = load_const("bias2", F32)
                tapTs = []
                for b in range(3):
                    tapT = bigpool.tile([128, 1024], F16, tag=f"tapT{b}", name=f"tapT{b}")
                    for f in range(8):
                        transp(accs[b][:, 128 * f:128 * (f + 1)],
                               tapT[:, 128 * f:128 * (f + 1)])
                    tapTs.append(tapT)
                for fg in range(8):
                    z2fg = wpool.tile([128, 128], F16, tag="z2fg", name="z2fg")
                    einsum_win(bw1, [tapTs[0][:, 128 * fg:128 * (fg + 1)],
                                     tapTs[1][:, 128 * fg:128 * (fg + 1)],
                                     tapTs[2][:, 128 * fg:128 * (fg + 1)]],
                               128, 128, z2fg[:], AF.Tanh, bias2[:, 0:1])
                    nc.sync.dma_start(out=z2T_loc[128 * fg:128 * (fg + 1), :],
                                      in_=z2fg[:])
            with nc.named_scope("ag_z2"):
                ag(z2T_loc.ap(), z2T_all.ap())

            # ====== z2n assembly (batch-sharded node-major) ==================
            z2n = bigpool.tile([128, 8 * 128], F16, tag="z2n", name="z2n")
            with nc.named_scope("z2n_asm"):
                z2i = cpool.tile([128, 8 * 8], I16, tag="z2i", name="z2i")
                nc.sync.dma_start(out=z2i[:], in_=ein["z2n_idx"][:, :])
                zb = wpool.tile([128, 1024], F16, tag="zb", name="zb")
                nc.gpsimd.dma_gather(
                    out_ap=zb[:].rearrange("p (c e) -> p c e", e=128),
                    in_ap=z2T_all[:, :],
                    idxs_ap=z2i[:],
                    num_idxs=1024, num_idxs_reg=1024, elem_size=128,
                    single_packet=False)
                for ci in range(8):
                    transp(zb[:, ci * 128:(ci + 1) * 128],
                           z2n[:, ci * 128:(ci + 1) * 128])

            # ================= LEVEL 2 (dense) =================
            with nc.named_scope("l2"):
                t1_l2 = bigpool.tile([128, 8 * 128], F16, tag="t1_l2", name="t1_l2")
                for half in range(2):
                    s2t = wlpool.tile([128, 4096], F16, tag="wld", name="wld")
                    nc.sync.dma_start(out=s2t[:], in_=ein["S2T"][:, 4096 * half:4096 * (half + 1)])
                    for dc in range(8):
                        ps = ppool.tile([128, 512], F32, tag="ps", name="ps")
                        for kk in range(4):
                            kc = half * 4 + kk
                            nc.tensor.matmul(
                                out=ps[:, :128],
                                lhsT=s2t[:, kk * 1024 + dc * 128: kk * 1024 + dc * 128 + 128],
                                rhs=z2n[:, kc * 128:(kc + 1) * 128],
                                start=(kk == 0), stop=(kk == 3))
                        if half == 0:
                            nc.scalar.activation(out=t1_l2[:, dc * 128:(dc + 1) * 128],
                                                 in_=ps[:, :128], func=AF.Copy)
                        else:
                            nc.vector.tensor_add(t1_l2[:, dc * 128:(dc + 1) * 128],
                                                 t1_l2[:, dc * 128:(dc + 1) * 128],
                                                 ps[:, :128])
                s2l2 = cpool.tile([128, 1024], F16, tag="s2l2", name="s2l2")
                nc.sync.dma_start(out=s2l2[:], in_=ein["S2l2T"][:, :])
                ps = ppool.tile([128, 512], F32, tag="ps", name="ps")
                for kc in range(8):
                    nc.tensor.matmul(out=ps[:, :128], lhsT=s2l2[:, kc * 128:(kc + 1) * 128],
                                     rhs=t1_l2[:, kc * 128:(kc + 1) * 128],
                                     start=(kc == 0), stop=(kc == 7))
                p2n_l2 = wpool.tile([128, 128], F16, tag="p2n_l2", name="p2n_l2")
                nc.scalar.activation(out=p2n_l2[:], in_=ps[:, :128], func=AF.Copy)
                pl2 = cpool.tile([128, 1024], F16, tag="pl2", name="pl2")
                nc.sync.dma_start(out=pl2[:], in_=ein["P_l2"][:, :])
                z2l2T = wpool.tile([128, 128], F16, tag="z2l2T", name="z2l2T")
                psg = ppool.tile([128, 512], F32, tag="ps", name="ps")
                for kc in range(8):
                    nc.tensor.matmul(out=psg[:, :128], lhsT=z2n[:, kc * 128:(kc + 1) * 128],
                                     rhs=pl2[:, kc * 128:(kc + 1) * 128],
                                     start=(kc == 0), stop=(kc == 7))
                nc.scalar.activation(out=z2l2T[:], in_=psg[:, :128], func=AF.Copy)
                t1l2T = wpool.tile([128, 128], F16, tag="t1l2T", name="t1l2T")
                psg2 = ppool.tile([128, 512], F32, tag="ps", name="ps")
                for kc in range(8):
                    nc.tensor.matmul(out=psg2[:, :128], lhsT=t1_l2[:, kc * 128:(kc + 1) * 128],
                                     rhs=pl2[:, kc * 128:(kc + 1) * 128],
                                     start=(kc == 0), stop=(kc == 7))
                nc.scalar.activation(out=t1l2T[:], in_=psg2[:, :128], func=AF.Copy)
                p2l2T = wpool.tile([128, 128], F16, tag="p2l2T", name="p2l2T")
                transp(p2n_l2[:], p2l2T[:])
                bw2 = [load_const(f"bigw2_{t}") for t in range(3)]
                bias3 = load_const("bias3", F32)
                z3T = wpool.tile([128, 128], F16, tag="z3T", name="z3T")
                einsum_win(bw2, [z2l2T[:], t1l2T[:], p2l2T[:]], 128, 128,
                           z3T[:], AF.Tanh, bias3[:, 0:1])
                z3n = wpool.tile([128, 128], F16, tag="z3n", name="z3n")
                transp(z3T[:], z3n[:])

            # ================= LEVEL 3 =================
            with nc.named_scope("l3"):
                s3t = cpool.tile([128, 128], F16, tag="s3t", name="s3t")
                nc.sync.dma_start(out=s3t[:], in_=ein["S3T"][:, :])
                bias4 = load_const("bias4", F32)
                bias5 = load_const("bias5", F32)

                def conv_l3(zn, zT, bw_pref, bias_t, func, keep):
                    t1T = wpool.tile([128, 128], F16, tag=keep + "t1T", name=keep + "t1T")
                    ps = ppool.tile([128, 512], F32, tag="ps", name="ps")
                    nc.tensor.matmul(out=ps[:, :128], lhsT=zn, rhs=s3t[:], start=True, stop=True)
                    nc.scalar.activation(out=t1T[:], in_=ps[:, :128], func=AF.Copy)
                    t1n_ = wpool.tile([128, 128], F16, tag=keep + "t1n", name=keep + "t1n")
                    transp(t1T[:], t1n_[:])
                    p2T_ = wpool.tile([128, 128], F16, tag=keep + "p2T", name=keep + "p2T")
                    ps2 = ppool.tile([128, 512], F32, tag="ps", name="ps")
                    nc.tensor.matmul(out=ps2[:, :128], lhsT=t1n_[:], rhs=s3t[:], start=True, stop=True)
                    nc.scalar.activation(out=p2T_[:], in_=ps2[:, :128], func=AF.Copy)
                    bw = [load_const(f"{bw_pref}_{t}") for t in range(3)]
                    outT = wpool.tile([128, 128], F16, tag=keep + "oT", name=keep + "oT")
                    einsum_win(bw, [zT, t1T[:], p2T_[:]], 128, 128, outT[:], func, bias_t[:, 0:1])
                    outn = wpool.tile([128, 128], F16, tag=keep + "on", name=keep + "on")
                    transp(outT[:], outn[:])
                    return outn, outT

                z4n, z4T = conv_l3(z3n[:], z3T[:], "bigw3", bias4, AF.Tanh, "c4")
                o5n, o5T = conv_l3(z4n[:], z4T[:], "bigw4", bias5, AF.Copy, "c5")

            # ================= MLP input assembly =================
            with nc.named_scope("mlp_in"):
                nc.sync.dma_start(
                    out=x_loc.ap().rearrange("b (n c) -> n b c", c=32),
                    in_=o5n[:].rearrange("n (b c) -> n b c", c=32))
                ag(x_loc.ap(), x_all.ap())
                xT_sb = bigpool.tile([32, 4096], F16, tag="xT_sb", name="xT_sb")
                nc.sync.dma_start(out=xT_sb[:], in_=x_all[:, :])
                act6 = bigpool.tile([128, 1024], F16, tag="act6", name="act6")
                for i in range(32):
                    transp(xT_sb[:, 128 * i:128 * (i + 1)], act6[:, 32 * i:32 * i + 32])

            # ================= MLP =================
            def mlp_layer(li, act_sb, out_sb):
                g_t = load_const(f"g{li}", F32)
                be_t = load_const(f"be{li}", F32)
                wt = wlpool.tile([128, 32 * 512], F16, tag="wld", name="wld")
                nc.sync.dma_start(out=wt[:, :8192], in_=ein[f"w{li}"][:, :8192])
                nc.sync.dma_start(out=wt[:, 8192:], in_=ein[f"w{li}"][:, 8192:])
                acc = apool.tile([128, 512], F32, tag="acc", name="acc")
                for k2 in range(32):
                    nc.tensor.matmul(out=acc[:32, :], lhsT=act_sb[:, 32 * k2:32 * k2 + 32],
                                     rhs=wt[:, 512 * k2:512 * (k2 + 1)],
                                     start=(k2 == 0), stop=(k2 == 31))
                hb = wpool.tile([32, 512], F16, tag="hb", name="hb")
                nc.scalar.activation(out=hb[:], in_=acc[:32, :], func=AF.Copy)
                for c in range(4):
                    hc = wpool.tile([128, 32], F16, tag="hc", name="hc")
                    transp(hb[:, 128 * c:128 * (c + 1)], hc[:])
                    s1 = wpool.tile([128, 1], F32, tag="b_s1", name="b_s1")
                    nc.vector.tensor_reduce(out=s1[:], in_=hc[:], axis=AX.X, op=ALU.add)
                    mu_ = wpool.tile([128, 1], F32, tag="b_mu", name="b_mu")
                    nc.vector.tensor_scalar_mul(mu_[:], s1[:], 1.0 / 32.0)
                    sq = wpool.tile([128, 32], F32, tag="b_sq", name="b_sq")
                    nc.vector.tensor_mul(sq[:], hc[:], hc[:])
                    s2_ = wpool.tile([128, 1], F32, tag="b_s2", name="b_s2")
                    nc.vector.tensor_reduce(out=s2_[:], in_=sq[:], axis=AX.X, op=ALU.add)
                    var = wpool.tile([128, 1], F32, tag="b_var", name="b_var")
                    nc.vector.scalar_tensor_tensor(out=var[:], in0=mu_[:], scalar=-1.0,
                                                   in1=mu_[:], op0=ALU.mult, op1=ALU.mult)
                    nc.vector.scalar_tensor_tensor(out=var[:], in0=s2_[:], scalar=1.0 / 32.0,
                                                   in1=var[:], op0=ALU.mult, op1=ALU.add)
                    sd = wpool.tile([128, 1], F32, tag="b_sd", name="b_sd")
                    nc.scalar.activation(out=sd[:], in_=var[:], func=AF.Sqrt, bias=eps_t[:, 0:1])
                    rs = wpool.tile([128, 1], F32, tag="b_rs", name="b_rs")
                    nc.vector.reciprocal(rs[:], sd[:])
                    a_ = wpool.tile([128, 1], F32, tag="b_a", name="b_a")
                    nc.vector.tensor_mul(a_[:], rs[:], g_t[:, c:c + 1])
                    sh = wpool.tile([128, 1], F32, tag="b_sh", name="b_sh")
                    nc.vector.scalar_tensor_tensor(out=sh[:], in0=mu_[:], scalar=-1.0,
                                                   in1=a_[:], op0=ALU.mult, op1=ALU.mult)
                    nc.vector.tensor_add(sh[:], sh[:], be_t[:, c:c + 1])
                    nc.scalar.activation(out=out_sb[:, 32 * c:32 * c + 32], in_=hc[:],
                                         func=AF.Relu, scale=a_[:, 0:1], bias=sh[:, 0:1])

            with nc.named_scope("mlp6"):
                h6 = bigpool.tile([128, 128], F16, tag="h6sb", name="h6sb")
                mlp_layer(6, act6, h6)
                nc.sync.dma_start(out=h6_loc.ap(), in_=h6[:])
                ag(h6_loc.ap(), h6_all.ap())
            with nc.named_scope("mlp7"):
                act7 = bigpool.tile([128, 1024], F16, tag="act7", name="act7")
                for r in range(8):
                    nc.sync.dma_start(out=act7[:, 128 * r:128 * (r + 1)],
                                      in_=h6_all[128 * r:128 * (r + 1), :])
                h7 = bigpool.tile([128, 128], F16, tag="h7sb", name="h7sb")
                mlp_layer(7, act7, h7)
                nc.sync.dma_start(out=h7_loc.ap(), in_=h7[:])
                ag(h7_loc.ap(), h7_all.ap())
            with nc.named_scope("mlp8"):
                act8 = bigpool.tile([128, 1024], F16, tag="act8", name="act8")
                for r in range(8):
                    nc.sync.dma_start(out=act8[:, 128 * r:128 * (r + 1)],
                                      in_=h7_all[128 * r:128 * (r + 1), :])
                h8 = bigpool.tile([128, 128], F16, tag="h8sb", name="h8sb")
                mlp_layer(8, act8, h8)

            with nc.named_scope("mlp9"):
                w9t = cpool.tile([128, 512], F16, tag="w9t", name="w9t")
                nc.sync.dma_start(out=w9t[:], in_=ein["w9"][:, :])
                acc9 = apool.tile([128, 512], F32, tag="acc", name="acc9")
                for c in range(4):
                    nc.tensor.matmul(out=acc9[:32, :128], lhsT=h8[:, 32 * c:32 * c + 32],
                                     rhs=w9t[:, 128 * c:128 * (c + 1)],
                                     start=(c == 0), stop=(c == 3))
                p9sb = wpool.tile([32, 128], F32, tag="p9sb", name="p9sb")
                nc.scalar.activation(out=p9sb[:], in_=acc9[:32, :128], func=AF.Copy)
                nc.sync.dma_start(out=p9_loc.ap(), in_=p9sb[:])
                ag(p9_loc.ap(), p9_all.ap())
                tot = wpool.tile([32, 128], F32, tag="f_tot", name="f_tot")
                nc.sync.dma_start(out=tot[:], in_=p9_all[0:32, :])
                for k in range(1, 8):
                    pk = wpool.tile([32, 128], F32, tag="f_pk", name="f_pk")
                    nc.sync.dma_start(out=pk[:], in_=p9_all[32 * k:32 * (k + 1), :])
                    nc.vector.tensor_add(tot[:], tot[:], pk[:])
                totT = wpool.tile([128, 32], F32, tag="f_totT", name="f_totT")
                pst = ppool.tile([128, 512], F32, tag="ps", name="pst")
                identf = cpool.tile([32, 32], F32, tag="identf", name="identf")
                nc.scalar.activation(out=identf[:], in_=ident[:32, :32], func=AF.Copy)
                nc.tensor.transpose(out=pst[:128, :32], in_=tot[:], identity=identf[:])
                nc.scalar.activation(out=totT[:], in_=pst[:128, :32], func=AF.Copy)
                s1 = wpool.tile([128, 1], F32, tag="f_s1", name="f_s1")
                nc.vector.tensor_reduce(out=s1[:], in_=totT[:], axis=AX.X, op=ALU.add)
                mu_ = wpool.tile([128, 1], F32, tag="f_mu", name="f_mu")
                nc.vector.tensor_scalar_mul(mu_[:], s1[:], 1.0 / 32.0)
                sq = wpool.tile([128, 32], F32, tag="f_sq", name="f_sq")
                nc.vector.tensor_mul(sq[:], totT[:], totT[:])
                s2_ = wpool.tile([128, 1], F32, tag="f_s2", name="f_s2")
                nc.vector.tensor_reduce(out=s2_[:], in_=sq[:], axis=AX.X, op=ALU.add)
                var = wpool.tile([128, 1], F32, tag="f_var", name="f_var")
                nc.vector.scalar_tensor_tensor(out=var[:], in0=mu_[:], scalar=-1.0,
                                               in1=mu_[:], op0=ALU.mult, op1=ALU.mult)
                nc.vector.scalar_tensor_tensor(out=var[:], in0=s2_[:], scalar=1.0 / 32.0,
                                               in1=var[:], op0=ALU.mult, op1=ALU.add)
                sdf = wpool.tile([128, 1], F32, tag="f_sd", name="f_sd")
                nc.scalar.activation(out=sdf[:], in_=var[:], func=AF.Sqrt, bias=eps_t[:, 0:1])
                rs = wpool.tile([128, 1], F32, tag="f_rs", name="f_rs")
                nc.vector.reciprocal(rs[:], sdf[:])
                neg = wpool.tile([128, 1], F32, tag="f_neg", name="f_neg")
                nc.vector.scalar_tensor_tensor(out=neg[:], in0=mu_[:], scalar=-1.0,
                                               in1=rs[:], op0=ALU.mult, op1=ALU.mult)
                outt = wpool.tile([128, 32], F32, tag="f_out", name="f_out")
                nc.scalar.activation(out=outt[:], in_=totT[:], func=AF.Identity,
                                     scale=rs[:, 0:1], bias=neg[:, 0:1])
                nc.sync.dma_start(out=out_mu[:, :], in_=outt[:])

    nc.compile()
    return nc


# ---------------------------------------------------------------- entry point
def kernel(**inputs) -> np.ndarray:
    per_core, meta = _host_prep(inputs)
    if "prog" not in _CACHE:
        _CACHE["prog"] = _build_nc(meta, per_core[0])
    nc = _CACHE["prog"]
    res = bass_utils.run_bass_kernel_spmd(nc, per_core, core_ids=list(range(NCORES)))
    return np.ascontiguousarray(res.results[0]["mu"].T)


# revision 26
# speedup vs baseline: 2.7402x; 1.4082x over previous
"""Trainium2 Bass kernel for nn_Encoder_base (5x ChebConv GNN + pool + MLP).

Distribution over 8 NeuronCores (all matmuls fp16, fp32 PSUM):
  - level 0: the two props the einsum needs (Tx1[l0] = S0[l0]@X0 and
    p2t = S0[l0]@S0@X0) are composed on the HOST into single operators on
    the input X0 (2-hop edge expansion M0 = S0[l0]*S0). Edge-major X0 rows
    are pregathered host-side -> the props are pure streaming selection
    matmuls: zero indirect DMA, zero full-graph AllGather.
  - level 1: stacked dense operator T = [P_l1; S1[l1]; M1=S1[l1]*S1]
    (3072 x 4096) applied to z1, dest-sharded (128 l1-positions/core, all
    1024 batch-features wide); einsum is dest-sharded too. Comm: one
    chunked z1 AllGather + one small z2 AllGather.
  - levels 2-3: batch-sharded (4 batches/core), dense-S matmuls,
    block-diagonal channel mixes in feature-major layout.
  - MLP: output-feature sharded (512 cols of W6/7/8, 512 rows of W9 per
    core); activations [128k,32] are the stationary lhsT, W streams as rhs;
    BatchNorm per-feature after a PE transpose; activations AllGathered.
"""
import numpy as np
import concourse.bass as bass
import concourse.bacc as bacc
import concourse.tile as tile
from concourse import mybir, bass_utils

F32 = mybir.dt.float32
F16 = mybir.dt.float16
I16 = mybir.dt.int16
AF = mybir.ActivationFunctionType
ALU = mybir.AluOpType
AX = mybir.AxisListType
RG = [list(range(8))]
NCORES = 8
N0, N1, N2, N3 = 16384, 4096, 1024, 128
EPS = 1e-5
H16 = np.float16

_CACHE = {}


# ---------------------------------------------------------------- host prep
def _prep_prop(row, col, we, n_dest, n_shard):
    """Sorted-by-dest edges -> 128-dest windows, 128-edge chunks, padded so
    chunk counts per window match across shards (one SPMD program).
    Emits per-chunk selection matrices sel[chunk, edge_local, dst_local]."""
    window = 128
    order = np.argsort(row, kind="stable")
    row, col, we = row[order], col[order], we[order]
    per = n_dest // n_shard
    nwin = per // window
    counts = np.zeros((n_shard, nwin), np.int64)
    lists = {}
    for s in range(n_shard):
        lo = s * per
        for wi in range(nwin):
            wlo = lo + wi * window
            a = np.searchsorted(row, wlo, side="left")
            b = np.searchsorted(row, wlo + window, side="left")
            lists[(s, wi)] = (row[a:b] - wlo, col[a:b], we[a:b])
            counts[s, wi] = (b - a + 127) // 128
    ncw = np.maximum(counts.max(axis=0), 1)
    C = int(ncw.sum())
    src = np.zeros((n_shard, C, 128), np.int64)
    sel = np.zeros((n_shard, C, 128, 128), np.float32)
    for s in range(n_shard):
        base = 0
        for wi in range(nwin):
            dl, cl, wl = lists[(s, wi)]
            n = len(dl)
            k = int(ncw[wi])
            src[s, base:base + k].reshape(-1)[:n] = cl
            ch = base + np.arange(n) // 128
            ep = np.arange(n) % 128
            sel[s, ch, ep, dl] = wl
            base += k
    return [int(x) for x in ncw], src, sel


def _edge_we(e, n):
    row, col = np.asarray(e[0], np.int64), np.asarray(e[1], np.int64)
    deg = np.bincount(row, minlength=n).astype(np.float32)
    dis = np.where(deg > 0, 1.0 / np.sqrt(np.maximum(deg, 1.0)), 0.0).astype(np.float32)
    return row, col, -(dis[row] * dis[col]).astype(np.float32)


def _sub_edges(row, col, we, pool_idx):
    order = np.argsort(row, kind="stable")
    row, col, we = row[order], col[order], we[order]
    starts = np.searchsorted(row, pool_idx, side="left")
    ends = np.searchsorted(row, pool_idx, side="right")
    nr, ncl, nw = [], [], []
    for i in range(len(pool_idx)):
        s, e = starts[i], ends[i]
        if e > s:
            nr.append(np.full(e - s, i, np.int64))
            ncl.append(col[s:e])
            nw.append(we[s:e])
    return np.concatenate(nr), np.concatenate(ncl), np.concatenate(nw)


def _twohop(ri, ci, wi, row, col, we, n):
    """(i,j,w1) sub-edges composed with full edges (j->k,w2): (i,k,w1*w2)."""
    order = np.argsort(row, kind="stable")
    rs, cs, ws = row[order], col[order], we[order]
    starts = np.searchsorted(rs, np.arange(n), side="left")
    ends = np.searchsorted(rs, np.arange(n), side="right")
    cnt = (ends - starts)[ci]
    I = np.repeat(ri, cnt)
    W1 = np.repeat(wi, cnt)
    base = np.repeat(starts[ci], cnt)
    within = np.arange(cnt.sum()) - np.repeat(np.cumsum(cnt) - cnt, cnt)
    offs = base + within
    return I, cs[offs], W1 * ws[offs]


def _dense_s(row, col, we, n, m):
    s = np.zeros((n, m), np.float32)
    np.add.at(s, (row, col), we)
    return s


def _tile_w(w, pack):
    """[K, M] -> [K//(128*pack) * 128, pack*M]: pack K-blocks side by side."""
    k, m = w.shape
    nb = k // 128
    t = w.reshape(nb // pack, pack, 128, m).transpose(0, 2, 1, 3)
    return np.ascontiguousarray(t.reshape((nb // pack) * 128, pack * m))


def _idx_tile(flat):
    """flat int idx list -> [128, len//16] int16 (16-part wrap, x8 replicas)."""
    return np.ascontiguousarray(
        np.tile(flat.astype(np.int16).reshape(-1, 16).T, (8, 1)))


def _chunk_tile(arr3):
    """[C, 128, W] -> [128, C*W] (chunk c at cols c*W..)."""
    C, _, W = arr3.shape
    return np.ascontiguousarray(
        arr3.transpose(1, 0, 2).reshape(128, C * W)).astype(H16)


def _host_prep(inputs):
    d = {k: np.asarray(v) for k, v in inputs.items()}
    x = d["x"].astype(np.float32)
    l0 = np.asarray(d["l0"], np.int64)
    l1 = np.asarray(d["l1"], np.int64)
    l2 = np.asarray(d["l2"], np.int64)

    X0 = np.ascontiguousarray(x.transpose(1, 0, 2).reshape(N0, 96))
    X0p = np.zeros((N0, 128), np.float32)
    X0p[:, :96] = X0
    X0l0T = np.ascontiguousarray(X0[l0].T)  # [96, 4096]

    # level-0 operators on X0: a = S0[l0] (tap1), m = S0[l0]@S0 (tap2)
    r0, c0, w0 = _edge_we(d["e0"], N0)
    r0s, c0s, w0s = _sub_edges(r0, c0, w0, l0)
    ncw_a, src_a, sel_a = _prep_prop(r0s, c0s, w0s, N1, NCORES)
    mI, mK, mW = _twohop(r0s, c0s, w0s, r0, c0, w0, N0)
    ncw_m, src_m, sel_m = _prep_prop(mI, mK, mW, N1, NCORES)

    # level-1 stacked operator T = [P_l1; S1[l1]; M1]
    r1, c1, w1 = _edge_we(d["e1"], N1)
    S1 = _dense_s(r1, c1, w1, N1, N1)
    r1s, c1s, w1s = _sub_edges(r1, c1, w1, l1)
    S1l1 = _dense_s(r1s, c1s, w1s, N2, N1)    # [1024, 4096]
    M1 = S1l1 @ S1                            # [1024, 4096]
    P_l1 = np.zeros((N2, N1), np.float32)
    P_l1[np.arange(N2), l1] = 1.0
    Tblocks = [P_l1, S1l1, M1]

    r2, c2, w2 = _edge_we(d["e2"], N2)
    S2 = _dense_s(r2, c2, w2, N2, N2)
    S2T = _tile_w(np.ascontiguousarray(S2.T), 8).astype(H16)       # [128, 8192]
    S2l2T = _tile_w(np.ascontiguousarray(S2[l2].T), 8).astype(H16)  # [128, 1024]
    P_l2 = np.zeros((N2, 128), np.float32)
    P_l2[l2, np.arange(128)] = 1.0
    P_l2 = _tile_w(P_l2, 8).astype(H16)                             # [128, 1024]

    r3, c3, w3 = _edge_we(d["e3"], N3)
    S3T = np.ascontiguousarray(_dense_s(r3, c3, w3, N3, N3).T).astype(H16)

    def wmod(W):
        return W[0] - W[2], W[1], 2.0 * W[2]

    Wm1 = wmod(d["Wc1"].astype(np.float32))
    Wm = [wmod(d[f"Wc{i}"].astype(np.float32)) for i in (2, 3, 4, 5)]
    eye4 = np.eye(4, dtype=np.float32)

    per_core = []
    for k in range(NCORES):
        m = {}
        m["identbf"] = np.eye(128, dtype=np.float32).astype(H16)
        m["epsv"] = np.full((128, 1), EPS, np.float32)
        m["warm"] = np.zeros((1, 8), np.float32)
        m["X0l0Tw"] = np.ascontiguousarray(
            X0l0T[:, 512 * k:512 * (k + 1)]).astype(H16)
        m["Xg_a"] = _chunk_tile(X0p[src_a[k]].astype(H16))
        m["sel_a"] = _chunk_tile(sel_a[k])
        m["Xg_m"] = _chunk_tile(X0p[src_m[k]].astype(H16))
        m["sel_m"] = _chunk_tile(sel_m[k])
        # stacked-T lhsT chunks: block b, k-chunk kk at cols (b*32+kk)*128
        tt = np.zeros((128, 96 * 128), np.float32)
        for b, blk in enumerate(Tblocks):
            bt = blk[128 * k:128 * (k + 1), :].T  # [4096, 128]
            for kk in range(32):
                tt[:, (b * 32 + kk) * 128:(b * 32 + kk + 1) * 128] = \
                    bt[128 * kk:128 * (kk + 1), :]
        m["Tt"] = tt.astype(H16)
        # z2n extraction: for node-chunk c, rows 1024c + 128k + j of z2T_all
        m["z2n_idx"] = _idx_tile(np.concatenate(
            [1024 * c + 128 * k + np.arange(128) for c in range(8)]))
        m["S2T"] = S2T
        m["S2l2T"] = S2l2T
        m["P_l2"] = P_l2
        m["S3T"] = S3T
        for g in range(8):
            for t in range(3):
                bw = np.zeros((96, 128), np.float32)
                for j in range(4):
                    bg = 4 * g + j
                    bw[3 * bg:3 * bg + 3, 32 * j:32 * j + 32] = Wm1[t]
                m[f"bigw0_{g}_{t}"] = bw.astype(H16)
        for lev in range(4):
            for t in range(3):
                m[f"bigw{lev + 1}_{t}"] = np.kron(eye4, Wm[lev][t]).astype(H16)
        for lev, nm in ((1, "b1"), (2, "b2"), (3, "b3"), (4, "b4"), (5, "b5")):
            m[f"bias{lev}"] = np.tile(d[nm].astype(np.float32), 4).reshape(128, 1)
        for li in (6, 7, 8):
            W = d[f"W{li}"].astype(np.float32)[:, 512 * k:512 * k + 512]
            m[f"w{li}"] = np.ascontiguousarray(
                W.reshape(32, 128, 512).transpose(1, 0, 2).reshape(128, 32 * 512)
            ).astype(H16)
            m[f"g{li}"] = np.ascontiguousarray(
                d[f"g{li}"].astype(np.float32)[512 * k:512 * k + 512].reshape(4, 128).T)
            m[f"be{li}"] = np.ascontiguousarray(
                d[f"be{li}"].astype(np.float32)[512 * k:512 * k + 512].reshape(4, 128).T)
        W9 = d["W9"].astype(np.float32)[512 * k:512 * k + 512]  # [512, 128]
        m["w9"] = np.ascontiguousarray(
            W9.reshape(4, 128, 128).transpose(1, 0, 2).reshape(128, 512)).astype(H16)
        per_core.append(m)

    meta = {"a": ncw_a, "m": ncw_m}
    return per_core, meta


# ---------------------------------------------------------------- device program
def _build_nc(meta, shapes):
    nc = bacc.Bacc("TRN2", target_bir_lowering=False, debug=False, num_devices=NCORES)
    ein = {}
    for name, arr in shapes.items():
        dt = {np.dtype(np.int16): I16, np.dtype(H16): F16,
              np.dtype(np.float32): F32}[arr.dtype]
        ein[name] = nc.dram_tensor(name, list(arr.shape), dt, kind="ExternalInput")
    out_mu = nc.dram_tensor("mu", [128, 32], F32, kind="ExternalOutput")

    warm_all = nc.dram_tensor("warm_all", [8, 8], F32)
    warm_loc = nc.dram_tensor("warm_loc", [1, 8], F32)
    z1_loc = nc.dram_tensor("z1_loc", [512, 1024], F16)
    z1_all = nc.dram_tensor("z1_all", [N1, 1024], F16)  # true node order
    z2T_loc = nc.dram_tensor("z2T_loc", [1024, 128], F16)
    z2T_all = nc.dram_tensor("z2T_all", [8192, 128], F16)
    x_loc = nc.dram_tensor("x_loc", [4, 4096], F16)
    x_all = nc.dram_tensor("x_all", [32, 4096], F16)
    h6_loc = nc.dram_tensor("h6_loc", [128, 128], F16)
    h6_all = nc.dram_tensor("h6_all", [1024, 128], F16)
    h7_loc = nc.dram_tensor("h7_loc", [128, 128], F16)
    h7_all = nc.dram_tensor("h7_all", [1024, 128], F16)
    p9_loc = nc.dram_tensor("p9_loc", [32, 128], F32)
    p9_all = nc.dram_tensor("p9_all", [256, 128], F32)

    def ag(loc_ap, all_ap):
        nc.gpsimd.collective_compute(
            "AllGather", ALU.bypass, replica_groups=RG,
            ins=[loc_ap.opt()], outs=[all_ap.opt()])

    with tile.TileContext(nc) as tc:
        with (
            tc.tile_pool(name="const", bufs=1) as cpool,
            tc.tile_pool(name="big", bufs=1) as bigpool,
            tc.tile_pool(name="work", bufs=3) as wpool,
            tc.tile_pool(name="wload", bufs=2) as wlpool,
            tc.tile_pool(name="psA", bufs=3, space="PSUM") as ppool,
            tc.tile_pool(name="psT", bufs=2, space="PSUM") as tpool,
            tc.tile_pool(name="psB", bufs=1, space="PSUM") as apool,
        ):
            ident = cpool.tile([128, 128], F16, tag="identbf", name="identbf")
            nc.sync.dma_start(out=ident[:], in_=ein["identbf"][:, :])
            eps_t = cpool.tile([128, 1], F32, tag="epsv", name="epsv")
            nc.sync.dma_start(out=eps_t[:], in_=ein["epsv"][:, :])

            def load_const(name, dt=F16):
                t = cpool.tile(list(shapes[name].shape), dt, tag=name)
                nc.sync.dma_start(out=t[:], in_=ein[name][:, :])
                return t

            GRP = 8

            def grp_load(pref, g0, gc, tag):
                sl = wpool.tile([128, GRP * 128], F16, tag=tag, name=tag, bufs=2)
                nc.sync.dma_start(out=sl[:, :gc * 128],
                                  in_=ein[pref][:, g0 * 128:(g0 + gc) * 128])
                return sl

            def transp(src_ap, dst_ap):
                p, f = src_ap.shape
                ps = tpool.tile([128, 128], F16, tag="tp", name="tp")
                nc.tensor.transpose(out=ps[:f, :p], in_=src_ap, identity=ident[:p, :p])
                nc.scalar.activation(out=dst_ap, in_=ps[:f, :p], func=AF.Copy)

            def einsum_win(bigw, taps, Din, width, out_ap, func, bias_ap):
                ps = ppool.tile([128, 512], F32, tag="ps", name="ps")
                for t in range(3):
                    nc.tensor.matmul(out=ps[:, :width], lhsT=bigw[t][:Din, :],
                                     rhs=taps[t], start=(t == 0), stop=(t == 2))
                f2 = AF.Identity if func == AF.Copy else func
                nc.scalar.activation(out=out_ap, in_=ps[:, :width], func=f2, bias=bias_ap)

            # warm up the CC ring while level-0 computes
            with nc.named_scope("warmup"):
                warm = wpool.tile([1, 8], F32, tag="warm", name="warm")
                nc.sync.dma_start(out=warm[:], in_=ein["warm"][:, :])
                nc.sync.dma_start(out=warm_loc.ap(), in_=warm[:])
                ag(warm_loc.ap(), warm_all.ap())

            # ====== LEVEL 0 props: tap1 = S0[l0]@X0, tap2 = (S0[l0]@S0)@X0 ===
            with nc.named_scope("l0_props"):
                Ca, Cm = sum(meta["a"]), sum(meta["m"])
                cura = {"g0": -1}
                curm = {"g0": -1}

                def get_grp(cur, cc, C, xg, sel, xtag, stag):
                    g0 = (cc // GRP) * GRP
                    if g0 != cur["g0"]:
                        gc = min(GRP, C - g0)
                        cur["g0"] = g0
                        cur["x"] = grp_load(xg, g0, gc, xtag)
                        cur["s"] = grp_load(sel, g0, gc, stag)
                    return cur, (cc - cur["g0"]) * 128

                tap1_sb = bigpool.tile([96, 512], F16, tag="tap1_sb", name="tap1_sb")
                tap2_sb = bigpool.tile([96, 512], F16, tag="tap2_sb", name="tap2_sb")
                ba, bm = 0, 0
                for wi in range(4):
                    psa = ppool.tile([128, 512], F32, tag="ps", name="ps")
                    for c in range(meta["a"][wi]):
                        g, lo = get_grp(cura, ba + c, Ca, "Xg_a", "sel_a", "xga", "sla")
                        nc.tensor.matmul(out=psa[:96, :128], lhsT=g["x"][:, lo:lo + 96],
                                         rhs=g["s"][:, lo:lo + 128],
                                         start=(c == 0), stop=(c == meta["a"][wi] - 1))
                    nc.scalar.activation(out=tap1_sb[:, wi * 128:(wi + 1) * 128],
                                         in_=psa[:96, :128], func=AF.Copy)
                    ba += meta["a"][wi]
                    psm = ppool.tile([128, 512], F32, tag="ps", name="ps")
                    for c in range(meta["m"][wi]):
                        g, lo = get_grp(curm, bm + c, Cm, "Xg_m", "sel_m", "xgm", "slm")
                        nc.tensor.matmul(out=psm[:96, :128], lhsT=g["x"][:, lo:lo + 96],
                                         rhs=g["s"][:, lo:lo + 128],
                                         start=(c == 0), stop=(c == meta["m"][wi] - 1))
                    nc.scalar.activation(out=tap2_sb[:, wi * 128:(wi + 1) * 128],
                                         in_=psm[:96, :128], func=AF.Copy)
                    bm += meta["m"][wi]

            # ============ LEVEL 0 einsum -> z1 (dest-sharded) ================
            with nc.named_scope("l0_einsum"):
                bias1 = load_const("bias1", F32)
                x0w = cpool.tile([96, 512], F16, tag="X0l0Tw", name="X0l0Tw")
                nc.sync.dma_start(out=x0w[:], in_=ein["X0l0Tw"][:, :])
                for g in range(8):
                    bw0 = [load_const(f"bigw0_{g}_{t}") for t in range(3)]
                    z1gT = wpool.tile([128, 512], F16, tag="z1Tw", name="z1Tw")
                    einsum_win(bw0, [x0w[:], tap1_sb[:], tap2_sb[:]], 96, 512,
                               z1gT[:], AF.Copy, bias1[:, 0:1])
                    for c in range(4):
                        t = wpool.tile([128, 128], F16, tag="z1nc", name="z1nc")
                        transp(z1gT[:, c * 128:(c + 1) * 128], t[:])
                        nc.sync.dma_start(
                            out=z1_loc[128 * c:128 * (c + 1), 128 * g:128 * (g + 1)],
                            in_=t[:])
            with nc.named_scope("ag_z1"):
                ag(z1_loc.ap(), z1_all.ap())

            # ====== LEVEL 1: taps = T @ z1, dest-sharded (128 pos x 1024) ====
            with nc.named_scope("l1_T"):
                tt = cpool.tile([128, 96 * 128], F16, tag="Tt", name="Tt")
                nc.sync.dma_start(out=tt[:, :6144], in_=ein["Tt"][:, :6144])
                nc.sync.dma_start(out=tt[:, 6144:], in_=ein["Tt"][:, 6144:])
                accs = [bigpool.tile([128, 1024], F16, tag=f"accT{b}", name=f"accT{b}")
                        for b in range(3)]
                for ks in range(8):  # superchunks of 4 k-chunks (512 nodes)
                    z1sc = wpool.tile([128, 4096], F16, tag="z1sc", name="z1sc", bufs=2)
                    nc.sync.dma_start(
                        out=z1sc[:].rearrange("p (q d) -> p q d", d=1024),
                        in_=z1_all.ap()[512 * ks:512 * (ks + 1)]
                            .rearrange("(q p) d -> p q d", p=128))
                    for b in range(3):
                        psa = ppool.tile([128, 512], F32, tag="ps", name="ps")
                        psb = ppool.tile([128, 512], F32, tag="ps", name="ps")
                        for q in range(4):
                            kk = ks * 4 + q
                            lh = tt[:, (b * 32 + kk) * 128:(b * 32 + kk + 1) * 128]
                            nc.tensor.matmul(out=psa[:, :512], lhsT=lh,
                                             rhs=z1sc[:, q * 1024:q * 1024 + 512],
                                             start=(q == 0), stop=(q == 3))
                            nc.tensor.matmul(out=psb[:, :512], lhsT=lh,
                                             rhs=z1sc[:, q * 1024 + 512:(q + 1) * 1024],
                                             start=(q == 0), stop=(q == 3))
                        if ks == 0:
                            nc.scalar.activation(out=accs[b][:, :512], in_=psa[:, :512],
                                                 func=AF.Copy)
                            nc.scalar.activation(out=accs[b][:, 512:], in_=psb[:, :512],
                                                 func=AF.Copy)
                        else:
                            nc.vector.tensor_add(accs[b][:, :512], accs[b][:, :512],
                                                 psa[:, :512])
                            nc.vector.tensor_add(accs[b][:, 512:], accs[b][:, 512:],
                                                 psb[:, :512])

            # ============ LEVEL 1 einsum (dest-sharded) -> z2T ===============
            with nc.named_scope("l1_einsum"):
                bw1 = [load_const(f"bigw1_{t}") for t in range(3)]
                bias2 = load_const("bias2", F32)
                tapTs = []
                for b in range(3):
                    tapT = bigpool.tile([128, 1024], F16, tag=f"tapT{b}", name=f"tapT{b}")
                    for f in range(8):
                        transp(accs[b][:, 128 * f:128 * (f + 1)],
                               tapT[:, 128 * f:128 * (f + 1)])
                    tapTs.append(tapT)
                for fg in range(8):
                    z2fg = wpool.tile([128, 128], F16, tag="z2fg", name="z2fg")
                    einsum_win(bw1, [tapTs[0][:, 128 * fg:128 * (fg + 1)],
                                     tapTs[1][:, 128 * fg:128 * (fg + 1)],
                                     tapTs[2][:, 128 * fg:128 * (fg + 1)]],
                               128, 128, z2fg[:], AF.Tanh, bias2[:, 0:1])
                    nc.sync.dma_start(out=z2T_loc[128 * fg:128 * (fg + 1), :],
                                      in_=z2fg[:])
            with nc.named_scope("ag_z2"):
                ag(z2T_loc.ap(), z2T_all.ap())

            # ====== z2n assembly (batch-sharded node-major) ==================
            z2n = bigpool.tile([128, 8 * 128], F16, tag="z2n", name="z2n")
            with nc.named_scope("z2n_asm"):
                z2i = cpool.tile([128, 8 * 8], I16, tag="z2i", name="z2i")
                nc.sync.dma_start(out=z2i[:], in_=ein["z2n_idx"][:, :])
                zb = wpool.tile([128, 1024], F16, tag="zb", name="zb")
                nc.gpsimd.dma_gather(
                    out_ap=zb[:].rearrange("p (c e) -> p c e", e=128),
                    in_ap=z2T_all[:, :],
                    idxs_ap=z2i[:],
                    num_idxs=1024, num_idxs_reg=1024, elem_size=128,
                    single_packet=False)
                for ci in range(8):
                    transp(zb[:, ci * 128:(ci + 1) * 128],
                           z2n[:, ci * 128:(ci + 1) * 128])

            # ================= LEVEL 2 (dense) =================
            with nc.named_scope("l2"):
                t1_l2 = bigpool.tile([128, 8 * 128], F16, tag="t1_l2", name="t1_l2")
                for half in range(2):
                    s2t = wlpool.tile([128, 4096], F16, tag="wld", name="wld")
                    nc.sync.dma_start(out=s2t[:], in_=ein["S2T"][:, 4096 * half:4096 * (half + 1)])
                    for dc in range(8):
                        ps = ppool.tile([128, 512], F32, tag="ps", name="ps")
                        for kk in range(4):
                            kc = half * 4 + kk
                            nc.tensor.matmul(
                                out=ps[:, :128],
                                lhsT=s2t[:, kk * 1024 + dc * 128: kk * 1024 + dc * 128 + 128],
                                rhs=z2n[:, kc * 128:(kc + 1) * 128],
                                start=(kk == 0), stop=(kk == 3))
                        if half == 0:
                            nc.scalar.activation(out=t1_l2[:, dc * 128:(dc + 1) * 128],
                                                 in_=ps[:, :128], func=AF.Copy)
                        else:
                            nc.vector.tensor_add(t1_l2[:, dc * 128:(dc + 1) * 128],
                                                 t1_l2[:, dc * 128:(dc + 1) * 128],
                                                 ps[:, :128])
                s2l2 = cpool.tile([128, 1024], F16, tag="s2l2", name="s2l2")
                nc.sync.dma_start(out=s2l2[:], in_=ein["S2l2T"][:, :])
                ps = ppool.tile([128, 512], F32, tag="ps", name="ps")
                for kc in range(8):
                    nc.tensor.matmul(out=ps[:, :128], lhsT=s2l2[:, kc * 128:(kc + 1) * 128],
                                     rhs=t1_l2[:, kc * 128:(kc + 1) * 128],
                                     start=(kc == 0), stop=(kc == 7))
                p2n_l2 = wpool.tile([128, 128], F16, tag="p2n_l2", name="p2n_l2")
                nc.scalar.activation(out=p2n_l2[:], in_=ps[:, :128], func=AF.Copy)
                pl2 = cpool.tile([128, 1024], F16, tag="pl2", name="pl2")
                nc.sync.dma_start(out=pl2[:], in_=ein["P_l2"][:, :])
                z2l2T = wpool.tile([128, 128], F16, tag="z2l2T", name="z2l2T")
                psg = ppool.tile([128, 512], F32, tag="ps", name="ps")
                for kc in range(8):
                    nc.tensor.matmul(out=psg[:, :128], lhsT=z2n[:, kc * 128:(kc + 1) * 128],
                                     rhs=pl2[:, kc * 128:(kc + 1) * 128],
                                     start=(kc == 0), stop=(kc == 7))
                nc.scalar.activation(out=z2l2T[:], in_=psg[:, :128], func=AF.Copy)
                t1l2T = wpool.tile([128, 128], F16, tag="t1l2T", name="t1l2T")
                psg2 = ppool.tile([128, 512], F32, tag="ps", name="ps")
                for kc in range(8):
                    nc.tensor.matmul(out=psg2[:, :128], lhsT=t1_l2[:, kc * 128:(kc + 1) * 128],
                                     rhs=pl2[:, kc * 128:(kc + 1) * 128],
                                     start=(kc == 0), stop=(kc == 7))
                nc.scalar.activation(out=t1l2T[:], in_=psg2[:, :128], func=AF.Copy)
                p2l2T = wpool.tile([128, 128], F16, tag="p2l2T", name="p2l2T")
                transp(p2n_l2[:], p2l2T[:])
                bw2 = [load_const(f"bigw2_{t}") for t in range(3)]
                bias3 = load_const("bias3", F32)
                z3T = wpool.tile([128, 128], F16, tag="z3T", name="z3T")
                einsum_win(bw2, [z2l2T[:], t1l2T[:], p2l2T[:]], 128, 128,
                           z3T[:], AF.Tanh, bias3[:, 0:1])
                z3n = wpool.tile([128, 128], F16, tag="z3n", name="z3n")
                transp(z3T[:], z3n[:])

            # ================= LEVEL 3 =================
            with nc.named_scope("l3"):
                s3t = cpool.tile([128, 128], F16, tag="s3t", name="s3t")
                nc.sync.dma_start(out=s3t[:], in_=ein["S3T"][:, :])
                bias4 = load_const("bias4", F32)
                bias5 = load_const("bias5", F32)

                def conv_l3(zn, zT, bw_pref, bias_t, func, keep):
                    t1T = wpool.tile([128, 128], F16, tag=keep + "t1T", name=keep + "t1T")
                    ps = ppool.tile([128, 512], F32, tag="ps", name="ps")
                    nc.tensor.matmul(out=ps[:, :128], lhsT=zn, rhs=s3t[:], start=True, stop=True)
                    nc.scalar.activation(out=t1T[:], in_=ps[:, :128], func=AF.Copy)
                    t1n_ = wpool.tile([128, 128], F16, tag=keep + "t1n", name=keep + "t1n")
                    transp(t1T[:], t1n_[:])
                    p2T_ = wpool.tile([128, 128], F16, tag=keep + "p2T", name=keep + "p2T")
                    ps2 = ppool.tile([128, 512], F32, tag="ps", name="ps")
                    nc.tensor.matmul(out=ps2[:, :128], lhsT=t1n_[:], rhs=s3t[:], start=True, stop=True)
                    nc.scalar.activation(out=p2T_[:], in_=ps2[:, :128], func=AF.Copy)
                    bw = [load_const(f"{bw_pref}_{t}") for t in range(3)]
                    outT = wpool.tile([128, 128], F16, tag=keep + "oT", name=keep + "oT")
                    einsum_win(bw, [zT, t1T[:], p2T_[:]], 128, 128, outT[:], func, bias_t[:, 0:1])
                    outn = wpool.tile([128, 128], F16, tag=keep + "on", name=keep + "on")
                    transp(outT[:], outn[:])
                    return outn, outT

                z4n, z4T = conv_l3(z3n[:], z3T[:], "bigw3", bias4, AF.Tanh, "c4")
                o5n, o5T = conv_l3(z4n[:], z4T[:], "bigw4", bias5, AF.Copy, "c5")

            # ================= MLP input assembly =================
            with nc.named_scope("mlp_in"):
                nc.sync.dma_start(
                    out=x_loc.ap().rearrange("b (n c) -> n b c", c=32),
                    in_=o5n[:].rearrange("n (b c) -> n b c", c=32))
                ag(x_loc.ap(), x_all.ap())
                xT_sb = bigpool.tile([32, 4096], F16, tag="xT_sb", name="xT_sb")
                nc.sync.dma_start(out=xT_sb[:], in_=x_all[:, :])
                act6 = bigpool.tile([128, 1024], F16, tag="act6", name="act6")
                for i in range(32):
                    transp(xT_sb[:, 128 * i:128 * (i + 1)], act6[:, 32 * i:32 * i + 32])

            # ================= MLP =================
            def mlp_layer(li, act_sb, out_sb):
                g_t = load_const(f"g{li}", F32)
                be_t = load_const(f"be{li}", F32)
                wt = wlpool.tile([128, 32 * 512], F16, tag="wld", name="wld")
                nc.sync.dma_start(out=wt[:, :8192], in_=ein[f"w{li}"][:, :8192])
                nc.sync.dma_start(out=wt[:, 8192:], in_=ein[f"w{li}"][:, 8192:])
                acc = apool.tile([128, 512], F32, tag="acc", name="acc")
                for k2 in range(32):
                    nc.tensor.matmul(out=acc[:32, :], lhsT=act_sb[:, 32 * k2:32 * k2 + 32],
                                     rhs=wt[:, 512 * k2:512 * (k2 + 1)],
                                     start=(k2 == 0), stop=(k2 == 31))
                hb = wpool.tile([32, 512], F16, tag="hb", name="hb")
                nc.scalar.activation(out=hb[:], in_=acc[:32, :], func=AF.Copy)
                for c in range(4):
                    hc = wpool.tile([128, 32], F16, tag="hc", name="hc")
                    transp(hb[:, 128 * c:128 * (c + 1)], hc[:])
                    s1 = wpool.tile([128, 1], F32, tag="b_s1", name="b_s1")
                    nc.vector.tensor_reduce(out=s1[:], in_=hc[:], axis=AX.X, op=ALU.add)
                    mu_ = wpool.tile([128, 1], F32, tag="b_mu", name="b_mu")
                    nc.vector.tensor_scalar_mul(mu_[:], s1[:], 1.0 / 32.0)
                    sq = wpool.tile([128, 32], F32, tag="b_sq", name="b_sq")
                    nc.vector.tensor_mul(sq[:], hc[:], hc[:])
                    s2_ = wpool.tile([128, 1], F32, tag="b_s2", name="b_s2")
                    nc.vector.tensor_reduce(out=s2_[:], in_=sq[:], axis=AX.X, op=ALU.add)
                    var = wpool.tile([128, 1], F32, tag="b_var", name="b_var")
                    nc.vector.scalar_tensor_tensor(out=var[:], in0=mu_[:], scalar=-1.0,
                                                   in1=mu_[:], op0=ALU.mult, op1=ALU.mult)
                    nc.vector.scalar_tensor_tensor(out=var[:], in0=s2_[:], scalar=1.0 / 32.0,
                                                   in1=var[:], op0=ALU.mult, op1=ALU.add)
                    sd = wpool.tile([128, 1], F32, tag="b_sd", name="b_sd")
                    nc.scalar.activation(out=sd[:], in_=var[:], func=AF.Sqrt, bias=eps_t[:, 0:1])
                    rs = wpool.tile([128, 1], F32, tag="b_rs", name="b_rs")
                    nc.vector.reciprocal(rs[:], sd[:])
                    a_ = wpool.tile([128, 1], F32, tag="b_a", name="b_a")
                    nc.vector.tensor_mul(a_[:], rs[:], g_t[:, c:c + 1])
                    sh = wpool.tile([128, 1], F32, tag="b_sh", name="b_sh")
                    nc.vector.scalar_tensor_tensor(out=sh[:], in0=mu_[:], scalar=-1.0,
                                                   in1=a_[:], op0=ALU.mult, op1=ALU.mult)
                    nc.vector.tensor_add(sh[:], sh[:], be_t[:, c:c + 1])
                    nc.scalar.activation(out=out_sb[:, 32 * c:32 * c + 32], in_=hc[:],
                                         func=AF.Relu, scale=a_[:, 0:1], bias=sh[:, 0:1])

            with nc.named_scope("mlp6"):
                h6 = bigpool.tile([128, 128], F16, tag="h6sb", name="h6sb")
                mlp_layer(6, act6, h6)
                nc.sync.dma_start(out=h6_loc.ap(), in_=h6[:])
                ag(h6_loc.ap(), h6_all.ap())
            with nc.named_scope("mlp7"):
                act7 = bigpool.tile([128, 1024], F16, tag="act7", name="act7")
                for r in range(8):
                    nc.sync.dma_start(out=act7[:, 128 * r:128 * (r + 1)],
                                      in_=h6_all[128 * r:128 * (r + 1), :])
                h7 = bigpool.tile([128, 128], F16, tag="h7sb", name="h7sb")
                mlp_layer(7, act7, h7)
                nc.sync.dma_start(out=h7_loc.ap(), in_=h7[:])
                ag(h7_loc.ap(), h7_all.ap())
            with nc.named_scope("mlp8"):
                act8 = bigpool.tile([128, 1024], F16, tag="act8", name="act8")
                for r in range(8):
                    nc.sync.dma_start(out=act8[:, 128 * r:128 * (r + 1)],
                                      in_=h7_all[128 * r:128 * (r + 1), :])
                h8 = bigpool.tile([128, 128], F16, tag="h8sb", name="h8sb")
                mlp_layer(8, act8, h8)

            with nc.named_scope("mlp9"):
                w9t = cpool.tile([128, 512], F16, tag="w9t", name="w9t")
                nc.sync.dma_start(out=w9t[:], in_=ein["w9"][:, :])
                acc9 = apool.tile([128, 512], F32, tag="acc", name="acc9")
                for c in range(4):
                    nc.tensor.matmul(out=acc9[:32, :128], lhsT=h8[:, 32 * c:32 * c + 32],
                                     rhs=w9t[:, 128 * c:128 * (c + 1)],
                                     start=(c == 0), stop=(c == 3))
                p9sb = wpool.tile([32, 128], F32, tag="p9sb", name="p9sb")
                nc.scalar.activation(out=p9sb[:], in_=acc9[:32, :128], func=AF.Copy)
                nc.sync.dma_start(out=p9_loc.ap(), in_=p9sb[:])
                ag(p9_loc.ap(), p9_all.ap())
                tot = wpool.tile([32, 128], F32, tag="f_tot", name="f_tot")
                nc.sync.dma_start(out=tot[:], in_=p9_all[0:32, :])
                for k in range(1, 8):
                    pk = wpool.tile([32, 128], F32, tag="f_pk", name="f_pk")
                    nc.sync.dma_start(out=pk[:], in_=p9_all[32 * k:32 * (k + 1), :])
                    nc.vector.tensor_add(tot[:], tot[:], pk[:])
                totT = wpool.tile([128, 32], F32, tag="f_totT", name="f_totT")
                pst = ppool.tile([128, 512], F32, tag="ps", name="pst")
                identf = cpool.tile([32, 32], F32, tag="identf", name="identf")
                nc.scalar.activation(out=identf[:], in_=ident[:32, :32], func=AF.Copy)
                nc.tensor.transpose(out=pst[:128, :32], in_=tot[:], identity=identf[:])
                nc.scalar.activation(out=totT[:], in_=pst[:128, :32], func=AF.Copy)
                s1 = wpool.tile([128, 1], F32, tag="f_s1", name="f_s1")
                nc.vector.tensor_reduce(out=s1[:], in_=totT[:], axis=AX.X, op=ALU.add)
                mu_ = wpool.tile([128, 1], F32, tag="f_mu", name="f_mu")
                nc.vector.tensor_scalar_mul(mu_[:], s1[:], 1.0 / 32.0)
                sq = wpool.tile([128, 32], F32, tag="f_sq", name="f_sq")
                nc.vector.tensor_mul(sq[:], totT[:], totT[:])
                s2_ = wpool.tile([128, 1], F32, tag="f_s2", name="f_s2")
                nc.vector.tensor_reduce(out=s2_[:], in_=sq[:], axis=AX.X, op=ALU.add)
                var = wpool.tile([128, 1], F32, tag="f_var", name="f_var")
                nc.vector.scalar_tensor_tensor(out=var[:], in0=mu_[:], scalar=-1.0,
                                               in1=mu_[:], op0=ALU.mult, op1=ALU.mult)
                nc.vector.scalar_tensor_tensor(out=var[:], in0=s2_[:], scalar=1.0 / 32.0,
                                               in1=var[:], op0=ALU.mult, op1=ALU.add)
                sdf = wpool.tile([128, 1], F32, tag="f_sd", name="f_sd")
                nc.scalar.activation(out=sdf[:], in_=var[:], func=AF.Sqrt, bias=eps_t[:, 0:1])
                rs = wpool.tile([128, 1], F32, tag="f_rs", name="f_rs")
                nc.vector.reciprocal(rs[:], sdf[:])
                neg = wpool.tile([128, 1], F32, tag="f_neg", name="f_neg")
                nc.vector.scalar_tensor_tensor(out=neg[:], in0=mu_[:], scalar=-1.0,
                                               in1=rs[:], op0=ALU.mult, op1=ALU.mult)
                outt = wpool.tile([128, 32], F32, tag="f_out", name="f_out")
                nc.scalar.activation(out=outt[:], in_=totT[:], func=AF.Identity,
                                     scale=rs[:, 0:1], bias=neg[:, 0:1])
                nc.sync.dma_start(out=out_mu[:, :], in_=outt[:])

    nc.compile()
    return nc


# ---------------------------------------------------------------- entry point
def kernel(**inputs) -> np.ndarray:
    per_core, meta = _host_prep(inputs)
    if "prog" not in _CACHE:
        _CACHE["prog"] = _build_nc(meta, per_core[0])
    nc = _CACHE["prog"]
    res = bass_utils.run_bass_kernel_spmd(nc, per_core, core_ids=list(range(NCORES)))
    return np.ascontiguousarray(res.results[0]["mu"].T)


# revision 27
# speedup vs baseline: 2.9797x; 1.0874x over previous
"""Trainium2 Bass kernel for nn_Encoder_base (5x ChebConv GNN + pool + MLP).

Distribution over 8 NeuronCores (all matmuls fp16, fp32 PSUM):
  - level 0: the two props the einsum needs (Tx1[l0] = S0[l0]@X0 and
    p2t = S0[l0]@S0@X0) are composed on the HOST into single operators on
    the input X0 (2-hop edge expansion M0 = S0[l0]*S0). Edge-major X0 rows
    are pregathered host-side -> the props are pure streaming selection
    matmuls: zero indirect DMA, zero full-graph AllGather.
  - level 1: stacked dense operator T = [P_l1; S1[l1]; M1=S1[l1]*S1]
    (3072 x 4096) applied to z1, dest-sharded (128 l1-positions/core, all
    1024 batch-features wide); einsum is dest-sharded too. Comm: one
    chunked z1 AllGather + one small z2 AllGather.
  - levels 2-3: batch-sharded (4 batches/core), dense-S matmuls,
    block-diagonal channel mixes in feature-major layout.
  - MLP: output-feature sharded (512 cols of W6/7/8, 512 rows of W9 per
    core); activations [128k,32] are the stationary lhsT, W streams as rhs;
    BatchNorm per-feature after a PE transpose; activations AllGathered.
"""
import numpy as np
import concourse.bass as bass
import concourse.bacc as bacc
import concourse.tile as tile
from concourse import mybir, bass_utils

F32 = mybir.dt.float32
F16 = mybir.dt.float16
I16 = mybir.dt.int16
AF = mybir.ActivationFunctionType
ALU = mybir.AluOpType
AX = mybir.AxisListType
RG = [list(range(8))]
NCORES = 8
N0, N1, N2, N3 = 16384, 4096, 1024, 128
EPS = 1e-5
H16 = np.float16

_CACHE = {}


# ---------------------------------------------------------------- host prep
def _prep_prop(row, col, we, n_dest, n_shard):
    """Sorted-by-dest edges -> 128-dest windows, 128-edge chunks, padded so
    chunk counts per window match across shards (one SPMD program).
    Emits per-chunk selection matrices sel[chunk, edge_local, dst_local]."""
    window = 128
    order = np.argsort(row, kind="stable")
    row, col, we = row[order], col[order], we[order]
    per = n_dest // n_shard
    nwin = per // window
    counts = np.zeros((n_shard, nwin), np.int64)
    lists = {}
    for s in range(n_shard):
        lo = s * per
        for wi in range(nwin):
            wlo = lo + wi * window
            a = np.searchsorted(row, wlo, side="left")
            b = np.searchsorted(row, wlo + window, side="left")
            lists[(s, wi)] = (row[a:b] - wlo, col[a:b], we[a:b])
            counts[s, wi] = (b - a + 127) // 128
    ncw = np.maximum(counts.max(axis=0), 1)
    C = int(ncw.sum())
    src = np.zeros((n_shard, C, 128), np.int64)
    sel = np.zeros((n_shard, C, 128, 128), np.float32)
    for s in range(n_shard):
        base = 0
        for wi in range(nwin):
            dl, cl, wl = lists[(s, wi)]
            n = len(dl)
            k = int(ncw[wi])
            src[s, base:base + k].reshape(-1)[:n] = cl
            ch = base + np.arange(n) // 128
            ep = np.arange(n) % 128
            sel[s, ch, ep, dl] = wl
            base += k
    return [int(x) for x in ncw], src, sel


def _edge_we(e, n):
    row, col = np.asarray(e[0], np.int64), np.asarray(e[1], np.int64)
    deg = np.bincount(row, minlength=n).astype(np.float32)
    dis = np.where(deg > 0, 1.0 / np.sqrt(np.maximum(deg, 1.0)), 0.0).astype(np.float32)
    return row, col, -(dis[row] * dis[col]).astype(np.float32)


def _sub_edges(row, col, we, pool_idx):
    order = np.argsort(row, kind="stable")
    row, col, we = row[order], col[order], we[order]
    starts = np.searchsorted(row, pool_idx, side="left")
    ends = np.searchsorted(row, pool_idx, side="right")
    nr, ncl, nw = [], [], []
    for i in range(len(pool_idx)):
        s, e = starts[i], ends[i]
        if e > s:
            nr.append(np.full(e - s, i, np.int64))
            ncl.append(col[s:e])
            nw.append(we[s:e])
    return np.concatenate(nr), np.concatenate(ncl), np.concatenate(nw)


def _twohop(ri, ci, wi, row, col, we, n):
    """(i,j,w1) sub-edges composed with full edges (j->k,w2): (i,k,w1*w2)."""
    order = np.argsort(row, kind="stable")
    rs, cs, ws = row[order], col[order], we[order]
    starts = np.searchsorted(rs, np.arange(n), side="left")
    ends = np.searchsorted(rs, np.arange(n), side="right")
    cnt = (ends - starts)[ci]
    I = np.repeat(ri, cnt)
    W1 = np.repeat(wi, cnt)
    base = np.repeat(starts[ci], cnt)
    within = np.arange(cnt.sum()) - np.repeat(np.cumsum(cnt) - cnt, cnt)
    offs = base + within
    return I, cs[offs], W1 * ws[offs]


def _dense_s(row, col, we, n, m):
    s = np.zeros((n, m), np.float32)
    np.add.at(s, (row, col), we)
    return s


def _tile_w(w, pack):
    """[K, M] -> [K//(128*pack) * 128, pack*M]: pack K-blocks side by side."""
    k, m = w.shape
    nb = k // 128
    t = w.reshape(nb // pack, pack, 128, m).transpose(0, 2, 1, 3)
    return np.ascontiguousarray(t.reshape((nb // pack) * 128, pack * m))


def _idx_tile(flat):
    """flat int idx list -> [128, len//16] int16 (16-part wrap, x8 replicas)."""
    return np.ascontiguousarray(
        np.tile(flat.astype(np.int16).reshape(-1, 16).T, (8, 1)))


def _chunk_tile(arr3):
    """[C, 128, W] -> [128, C*W] (chunk c at cols c*W..)."""
    C, _, W = arr3.shape
    return np.ascontiguousarray(
        arr3.transpose(1, 0, 2).reshape(128, C * W)).astype(H16)


def _host_prep(inputs):
    d = {k: np.asarray(v) for k, v in inputs.items()}
    x = d["x"].astype(np.float32)
    l0 = np.asarray(d["l0"], np.int64)
    l1 = np.asarray(d["l1"], np.int64)
    l2 = np.asarray(d["l2"], np.int64)

    X0 = np.ascontiguousarray(x.transpose(1, 0, 2).reshape(N0, 96))
    X0p = np.zeros((N0, 128), np.float32)
    X0p[:, :96] = X0
    X0l0T = np.ascontiguousarray(X0[l0].T)  # [96, 4096]

    # level-0 operators on X0: a = S0[l0] (tap1), m = S0[l0]@S0 (tap2)
    r0, c0, w0 = _edge_we(d["e0"], N0)
    r0s, c0s, w0s = _sub_edges(r0, c0, w0, l0)
    ncw_a, src_a, sel_a = _prep_prop(r0s, c0s, w0s, N1, NCORES)
    mI, mK, mW = _twohop(r0s, c0s, w0s, r0, c0, w0, N0)
    ncw_m, src_m, sel_m = _prep_prop(mI, mK, mW, N1, NCORES)

    # level-1 stacked operator T = [P_l1; S1[l1]; M1]
    r1, c1, w1 = _edge_we(d["e1"], N1)
    S1 = _dense_s(r1, c1, w1, N1, N1)
    r1s, c1s, w1s = _sub_edges(r1, c1, w1, l1)
    S1l1 = _dense_s(r1s, c1s, w1s, N2, N1)    # [1024, 4096]
    M1 = S1l1 @ S1                            # [1024, 4096]
    P_l1 = np.zeros((N2, N1), np.float32)
    P_l1[np.arange(N2), l1] = 1.0
    Tblocks = [P_l1, S1l1, M1]

    r2, c2, w2 = _edge_we(d["e2"], N2)
    S2 = _dense_s(r2, c2, w2, N2, N2)
    S2T = _tile_w(np.ascontiguousarray(S2.T), 8).astype(H16)       # [128, 8192]
    S2l2T = _tile_w(np.ascontiguousarray(S2[l2].T), 8).astype(H16)  # [128, 1024]
    P_l2 = np.zeros((N2, 128), np.float32)
    P_l2[l2, np.arange(128)] = 1.0
    P_l2 = _tile_w(P_l2, 8).astype(H16)                             # [128, 1024]

    r3, c3, w3 = _edge_we(d["e3"], N3)
    S3T = np.ascontiguousarray(_dense_s(r3, c3, w3, N3, N3).T).astype(H16)

    def wmod(W):
        return W[0] - W[2], W[1], 2.0 * W[2]

    Wm1 = wmod(d["Wc1"].astype(np.float32))
    Wm = [wmod(d[f"Wc{i}"].astype(np.float32)) for i in (2, 3, 4, 5)]
    eye4 = np.eye(4, dtype=np.float32)

    per_core = []
    for k in range(NCORES):
        m = {}
        m["identbf"] = np.eye(128, dtype=np.float32).astype(H16)
        m["epsv"] = np.full((128, 1), EPS, np.float32)
        m["warm"] = np.zeros((1, 8), np.float32)
        m["X0l0Tw"] = np.ascontiguousarray(
            X0l0T[:, 512 * k:512 * (k + 1)]).astype(H16)
        m["Xg_a"] = _chunk_tile(X0p[src_a[k]].astype(H16))
        m["sel_a"] = _chunk_tile(sel_a[k])
        m["Xg_m"] = _chunk_tile(X0p[src_m[k]].astype(H16))
        m["sel_m"] = _chunk_tile(sel_m[k])
        # stacked-T lhsT chunks: block b, k-chunk kk at cols (b*32+kk)*128
        tt = np.zeros((128, 96 * 128), np.float32)
        for b, blk in enumerate(Tblocks):
            bt = blk[128 * k:128 * (k + 1), :].T  # [4096, 128]
            for kk in range(32):
                tt[:, (b * 32 + kk) * 128:(b * 32 + kk + 1) * 128] = \
                    bt[128 * kk:128 * (kk + 1), :]
        m["Tt"] = tt.astype(H16)
        # z2n extraction: for node-chunk c, rows 1024c + 128k + j of z2T_all
        m["z2n_idx"] = _idx_tile(np.concatenate(
            [1024 * c + 128 * k + np.arange(128) for c in range(8)]))
        m["S2T"] = S2T
        m["S2l2T"] = S2l2T
        m["P_l2"] = P_l2
        m["S3T"] = S3T
        for g in range(8):
            for t in range(3):
                bw = np.zeros((96, 128), np.float32)
                for j in range(4):
                    bg = 4 * g + j
                    bw[3 * bg:3 * bg + 3, 32 * j:32 * j + 32] = Wm1[t]
                m[f"bigw0_{g}_{t}"] = bw.astype(H16)
        for lev in range(4):
            for t in range(3):
                m[f"bigw{lev + 1}_{t}"] = np.kron(eye4, Wm[lev][t]).astype(H16)
        for lev, nm in ((1, "b1"), (2, "b2"), (3, "b3"), (4, "b4"), (5, "b5")):
            m[f"bias{lev}"] = np.tile(d[nm].astype(np.float32), 4).reshape(128, 1)
        for li in (6, 7, 8):
            W = d[f"W{li}"].astype(np.float32)[:, 512 * k:512 * k + 512]
            m[f"w{li}"] = np.ascontiguousarray(
                W.reshape(32, 128, 512).transpose(1, 0, 2).reshape(128, 32 * 512)
            ).astype(H16)
            m[f"g{li}"] = np.ascontiguousarray(
                d[f"g{li}"].astype(np.float32)[512 * k:512 * k + 512].reshape(4, 128).T)
            m[f"be{li}"] = np.ascontiguousarray(
                d[f"be{li}"].astype(np.float32)[512 * k:512 * k + 512].reshape(4, 128).T)
        W9 = d["W9"].astype(np.float32)[512 * k:512 * k + 512]  # [512, 128]
        m["w9"] = np.ascontiguousarray(
            W9.reshape(4, 128, 128).transpose(1, 0, 2).reshape(128, 512)).astype(H16)
        per_core.append(m)

    meta = {"a": ncw_a, "m": ncw_m}
    return per_core, meta


# ---------------------------------------------------------------- device program
def _build_nc(meta, shapes):
    nc = bacc.Bacc("TRN2", target_bir_lowering=False, debug=False, num_devices=NCORES)
    ein = {}
    for name, arr in shapes.items():
        dt = {np.dtype(np.int16): I16, np.dtype(H16): F16,
              np.dtype(np.float32): F32}[arr.dtype]
        ein[name] = nc.dram_tensor(name, list(arr.shape), dt, kind="ExternalInput")
    out_mu = nc.dram_tensor("mu", [128, 32], F32, kind="ExternalOutput")

    warm_all = nc.dram_tensor("warm_all", [8, 8], F32)
    warm_loc = nc.dram_tensor("warm_loc", [1, 8], F32)
    z1c = [nc.dram_tensor(f"z1c_{i}", [128, 1024], F16) for i in range(4)]
    z1ag = [nc.dram_tensor(f"z1ag_{i}", [1024, 1024], F16) for i in range(4)]
    z2T_loc = nc.dram_tensor("z2T_loc", [1024, 128], F16)
    z2T_all = nc.dram_tensor("z2T_all", [8192, 128], F16)
    x_loc = nc.dram_tensor("x_loc", [4, 4096], F16)
    x_all = nc.dram_tensor("x_all", [32, 4096], F16)
    h6_loc = nc.dram_tensor("h6_loc", [128, 128], F16)
    h6_all = nc.dram_tensor("h6_all", [1024, 128], F16)
    h7_loc = nc.dram_tensor("h7_loc", [128, 128], F16)
    h7_all = nc.dram_tensor("h7_all", [1024, 128], F16)
    p9_loc = nc.dram_tensor("p9_loc", [32, 128], F32)
    p9_all = nc.dram_tensor("p9_all", [256, 128], F32)

    def ag(loc_ap, all_ap):
        nc.gpsimd.collective_compute(
            "AllGather", ALU.bypass, replica_groups=RG,
            ins=[loc_ap.opt()], outs=[all_ap.opt()])

    with tile.TileContext(nc) as tc:
        with (
            tc.tile_pool(name="const", bufs=1) as cpool,
            tc.tile_pool(name="big", bufs=1) as bigpool,
            tc.tile_pool(name="work", bufs=3) as wpool,
            tc.tile_pool(name="wload", bufs=2) as wlpool,
            tc.tile_pool(name="psA", bufs=3, space="PSUM") as ppool,
            tc.tile_pool(name="psT", bufs=2, space="PSUM") as tpool,
            tc.tile_pool(name="psB", bufs=1, space="PSUM") as apool,
        ):
            ident = cpool.tile([128, 128], F16, tag="identbf", name="identbf")
            nc.sync.dma_start(out=ident[:], in_=ein["identbf"][:, :])
            eps_t = cpool.tile([128, 1], F32, tag="epsv", name="epsv")
            nc.sync.dma_start(out=eps_t[:], in_=ein["epsv"][:, :])

            def load_const(name, dt=F16):
                t = cpool.tile(list(shapes[name].shape), dt, tag=name)
                nc.sync.dma_start(out=t[:], in_=ein[name][:, :])
                return t

            GRP = 8

            def grp_load(pref, g0, gc, tag):
                sl = wpool.tile([128, GRP * 128], F16, tag=tag, name=tag, bufs=2)
                nc.sync.dma_start(out=sl[:, :gc * 128],
                                  in_=ein[pref][:, g0 * 128:(g0 + gc) * 128])
                return sl

            def transp(src_ap, dst_ap):
                p, f = src_ap.shape
                ps = tpool.tile([128, 128], F16, tag="tp", name="tp")
                nc.tensor.transpose(out=ps[:f, :p], in_=src_ap, identity=ident[:p, :p])
                nc.scalar.activation(out=dst_ap, in_=ps[:f, :p], func=AF.Copy)

            def einsum_win(bigw, taps, Din, width, out_ap, func, bias_ap):
                ps = ppool.tile([128, 512], F32, tag="ps", name="ps")
                for t in range(3):
                    nc.tensor.matmul(out=ps[:, :width], lhsT=bigw[t][:Din, :],
                                     rhs=taps[t], start=(t == 0), stop=(t == 2))
                f2 = AF.Identity if func == AF.Copy else func
                nc.scalar.activation(out=out_ap, in_=ps[:, :width], func=f2, bias=bias_ap)

            # warm up the CC ring while level-0 computes
            with nc.named_scope("warmup"):
                warm = wpool.tile([1, 8], F32, tag="warm", name="warm")
                nc.sync.dma_start(out=warm[:], in_=ein["warm"][:, :])
                nc.sync.dma_start(out=warm_loc.ap(), in_=warm[:])
                ag(warm_loc.ap(), warm_all.ap())

            # ====== LEVEL 0: per-window pipeline of props -> einsum -> AG ====
            # tap1 = S0[l0]@X0, tap2 = (S0[l0]@S0)@X0, then the channel-mix
            # einsum for window wi immediately, then AllGather that window.
            with nc.named_scope("l0"):
                Ca, Cm = sum(meta["a"]), sum(meta["m"])
                cura = {"g0": -1}
                curm = {"g0": -1}

                def get_grp(cur, cc, C, xg, sel, xtag, stag):
                    g0 = (cc // GRP) * GRP
                    if g0 != cur["g0"]:
                        gc = min(GRP, C - g0)
                        cur["g0"] = g0
                        cur["x"] = grp_load(xg, g0, gc, xtag)
                        cur["s"] = grp_load(sel, g0, gc, stag)
                    return cur, (cc - cur["g0"]) * 128

                bias1 = load_const("bias1", F32)
                x0w = cpool.tile([96, 512], F16, tag="X0l0Tw", name="X0l0Tw")
                nc.sync.dma_start(out=x0w[:], in_=ein["X0l0Tw"][:, :])
                bw0g = [[load_const(f"bigw0_{g}_{t}") for t in range(3)]
                        for g in range(8)]
                tap1_sb = bigpool.tile([96, 512], F16, tag="tap1_sb", name="tap1_sb")
                tap2_sb = bigpool.tile([96, 512], F16, tag="tap2_sb", name="tap2_sb")
                ba, bm = 0, 0
                for wi in range(4):
                    psa = ppool.tile([128, 512], F32, tag="ps", name="ps")
                    for c in range(meta["a"][wi]):
                        g, lo = get_grp(cura, ba + c, Ca, "Xg_a", "sel_a", "xga", "sla")
                        nc.tensor.matmul(out=psa[:96, :128], lhsT=g["x"][:, lo:lo + 96],
                                         rhs=g["s"][:, lo:lo + 128],
                                         start=(c == 0), stop=(c == meta["a"][wi] - 1))
                    nc.scalar.activation(out=tap1_sb[:, wi * 128:(wi + 1) * 128],
                                         in_=psa[:96, :128], func=AF.Copy)
                    ba += meta["a"][wi]
                    psm = ppool.tile([128, 512], F32, tag="ps", name="ps")
                    for c in range(meta["m"][wi]):
                        g, lo = get_grp(curm, bm + c, Cm, "Xg_m", "sel_m", "xgm", "slm")
                        nc.tensor.matmul(out=psm[:96, :128], lhsT=g["x"][:, lo:lo + 96],
                                         rhs=g["s"][:, lo:lo + 128],
                                         start=(c == 0), stop=(c == meta["m"][wi] - 1))
                    nc.scalar.activation(out=tap2_sb[:, wi * 128:(wi + 1) * 128],
                                         in_=psm[:96, :128], func=AF.Copy)
                    bm += meta["m"][wi]
                    for g in range(8):
                        z1gT = wpool.tile([128, 128], F16, tag="z1Tw", name="z1Tw")
                        einsum_win(bw0g[g],
                                   [x0w[:, wi * 128:(wi + 1) * 128],
                                    tap1_sb[:96, wi * 128:(wi + 1) * 128],
                                    tap2_sb[:96, wi * 128:(wi + 1) * 128]],
                                   96, 128, z1gT[:], AF.Copy, bias1[:, 0:1])
                        t = wpool.tile([128, 128], F16, tag="z1nc", name="z1nc")
                        transp(z1gT[:], t[:])
                        nc.sync.dma_start(
                            out=z1c[wi][:, 128 * g:128 * (g + 1)], in_=t[:])
                    with nc.named_scope(f"agz1_{wi}"):
                        ag(z1c[wi].ap(), z1ag[wi].ap())

            # ====== LEVEL 1: taps = T @ z1, dest-sharded (128 pos x 1024) ====
            with nc.named_scope("l1_T"):
                tt = cpool.tile([128, 96 * 128], F16, tag="Tt", name="Tt")
                nc.sync.dma_start(out=tt[:, :6144], in_=ein["Tt"][:, :6144])
                nc.sync.dma_start(out=tt[:, 6144:], in_=ein["Tt"][:, 6144:])
                accs = [bigpool.tile([128, 1024], F16, tag=f"accT{b}", name=f"accT{b}")
                        for b in range(3)]
                for part in range(4):
                    for h in range(2):
                        z1sc = wpool.tile([128, 4096], F16, tag="z1sc", name="z1sc", bufs=2)
                        nc.sync.dma_start(
                            out=z1sc[:].rearrange("p (q d) -> p q d", d=1024),
                            in_=z1ag[part].ap()[512 * h:512 * (h + 1)]
                                .rearrange("(q p) d -> p q d", p=128))
                        for b in range(3):
                            psa = ppool.tile([128, 512], F32, tag="ps", name="ps")
                            psb = ppool.tile([128, 512], F32, tag="ps", name="ps")
                            for q in range(4):
                                kk = 4 * (4 * h + q) + part
                                lh = tt[:, (b * 32 + kk) * 128:(b * 32 + kk + 1) * 128]
                                nc.tensor.matmul(out=psa[:, :512], lhsT=lh,
                                                 rhs=z1sc[:, q * 1024:q * 1024 + 512],
                                                 start=(q == 0), stop=(q == 3))
                                nc.tensor.matmul(out=psb[:, :512], lhsT=lh,
                                                 rhs=z1sc[:, q * 1024 + 512:(q + 1) * 1024],
                                                 start=(q == 0), stop=(q == 3))
                            if part == 0 and h == 0:
                                nc.scalar.activation(out=accs[b][:, :512], in_=psa[:, :512],
                                                     func=AF.Copy)
                                nc.scalar.activation(out=accs[b][:, 512:], in_=psb[:, :512],
                                                     func=AF.Copy)
                            else:
                                nc.vector.tensor_add(accs[b][:, :512], accs[b][:, :512],
                                                     psa[:, :512])
                                nc.vector.tensor_add(accs[b][:, 512:], accs[b][:, 512:],
                                                     psb[:, :512])

            # ============ LEVEL 1 einsum (dest-sharded) -> z2T ===============
            with nc.named_scope("l1_einsum"):
                bw1 = [load_const(f"bigw1_{t}") for t in range(3)]
                bias2 = load_const("bias2", F32)
                tapTs = []
                for b in range(3):
                    tapT = bigpool.tile([128, 1024], F16, tag=f"tapT{b}", name=f"tapT{b}")
                    for f in range(8):
                        transp(accs[b][:, 128 * f:128 * (f + 1)],
                               tapT[:, 128 * f:128 * (f + 1)])
                    tapTs.append(tapT)
                for fg in range(8):
                    z2fg = wpool.tile([128, 128], F16, tag="z2fg", name="z2fg")
                    einsum_win(bw1, [tapTs[0][:, 128 * fg:128 * (fg + 1)],
                                     tapTs[1][:, 128 * fg:128 * (fg + 1)],
                                     tapTs[2][:, 128 * fg:128 * (fg + 1)]],
                               128, 128, z2fg[:], AF.Tanh, bias2[:, 0:1])
                    nc.sync.dma_start(out=z2T_loc[128 * fg:128 * (fg + 1), :],
                                      in_=z2fg[:])
            with nc.named_scope("ag_z2"):
                ag(z2T_loc.ap(), z2T_all.ap())

            # ====== z2n assembly (batch-sharded node-major) ==================
            z2n = bigpool.tile([128, 8 * 128], F16, tag="z2n", name="z2n")
            with nc.named_scope("z2n_asm"):
                z2i = cpool.tile([128, 8 * 8], I16, tag="z2i", name="z2i")
                nc.sync.dma_start(out=z2i[:], in_=ein["z2n_idx"][:, :])
                zb = wpool.tile([128, 1024], F16, tag="zb", name="zb")
                nc.gpsimd.dma_gather(
                    out_ap=zb[:].rearrange("p (c e) -> p c e", e=128),
                    in_ap=z2T_all[:, :],
                    idxs_ap=z2i[:],
                    num_idxs=1024, num_idxs_reg=1024, elem_size=128,
                    single_packet=False)
                for ci in range(8):
                    transp(zb[:, ci * 128:(ci + 1) * 128],
                           z2n[:, ci * 128:(ci + 1) * 128])

            # ================= LEVEL 2 (dense) =================
            with nc.named_scope("l2"):
                t1_l2 = bigpool.tile([128, 8 * 128], F16, tag="t1_l2", name="t1_l2")
                for half in range(2):
                    s2t = wlpool.tile([128, 4096], F16, tag="wld", name="wld")
                    nc.sync.dma_start(out=s2t[:], in_=ein["S2T"][:, 4096 * half:4096 * (half + 1)])
                    for dc in range(8):
                        ps = ppool.tile([128, 512], F32, tag="ps", name="ps")
                        for kk in range(4):
                            kc = half * 4 + kk
                            nc.tensor.matmul(
                                out=ps[:, :128],
                                lhsT=s2t[:, kk * 1024 + dc * 128: kk * 1024 + dc * 128 + 128],
                                rhs=z2n[:, kc * 128:(kc + 1) * 128],
                                start=(kk == 0), stop=(kk == 3))
                        if half == 0:
                            nc.scalar.activation(out=t1_l2[:, dc * 128:(dc + 1) * 128],
                                                 in_=ps[:, :128], func=AF.Copy)
                        else:
                            nc.vector.tensor_add(t1_l2[:, dc * 128:(dc + 1) * 128],
                                                 t1_l2[:, dc * 128:(dc + 1) * 128],
                                                 ps[:, :128])
                s2l2 = cpool.tile([128, 1024], F16, tag="s2l2", name="s2l2")
                nc.sync.dma_start(out=s2l2[:], in_=ein["S2l2T"][:, :])
                ps = ppool.tile([128, 512], F32, tag="ps", name="ps")
                for kc in range(8):
                    nc.tensor.matmul(out=ps[:, :128], lhsT=s2l2[:, kc * 128:(kc + 1) * 128],
                                     rhs=t1_l2[:, kc * 128:(kc + 1) * 128],
                                     start=(kc == 0), stop=(kc == 7))
                p2n_l2 = wpool.tile([128, 128], F16, tag="p2n_l2", name="p2n_l2")
                nc.scalar.activation(out=p2n_l2[:], in_=ps[:, :128], func=AF.Copy)
                pl2 = cpool.tile([128, 1024], F16, tag="pl2", name="pl2")
                nc.sync.dma_start(out=pl2[:], in_=ein["P_l2"][:, :])
                z2l2T = wpool.tile([128, 128], F16, tag="z2l2T", name="z2l2T")
                psg = ppool.tile([128, 512], F32, tag="ps", name="ps")
                for kc in range(8):
                    nc.tensor.matmul(out=psg[:, :128], lhsT=z2n[:, kc * 128:(kc + 1) * 128],
                                     rhs=pl2[:, kc * 128:(kc + 1) * 128],
                                     start=(kc == 0), stop=(kc == 7))
                nc.scalar.activation(out=z2l2T[:], in_=psg[:, :128], func=AF.Copy)
                t1l2T = wpool.tile([128, 128], F16, tag="t1l2T", name="t1l2T")
                psg2 = ppool.tile([128, 512], F32, tag="ps", name="ps")
                for kc in range(8):
                    nc.tensor.matmul(out=psg2[:, :128], lhsT=t1_l2[:, kc * 128:(kc + 1) * 128],
                                     rhs=pl2[:, kc * 128:(kc + 1) * 128],
                                     start=(kc == 0), stop=(kc == 7))
                nc.scalar.activation(out=t1l2T[:], in_=psg2[:, :128], func=AF.Copy)
                p2l2T = wpool.tile([128, 128], F16, tag="p2l2T", name="p2l2T")
                transp(p2n_l2[:], p2l2T[:])
                bw2 = [load_const(f"bigw2_{t}") for t in range(3)]
                bias3 = load_const("bias3", F32)
                z3T = wpool.tile([128, 128], F16, tag="z3T", name="z3T")
                einsum_win(bw2, [z2l2T[:], t1l2T[:], p2l2T[:]], 128, 128,
                           z3T[:], AF.Tanh, bias3[:, 0:1])
                z3n = wpool.tile([128, 128], F16, tag="z3n", name="z3n")
                transp(z3T[:], z3n[:])

            # ================= LEVEL 3 =================
            with nc.named_scope("l3"):
                s3t = cpool.tile([128, 128], F16, tag="s3t", name="s3t")
                nc.sync.dma_start(out=s3t[:], in_=ein["S3T"][:, :])
                bias4 = load_const("bias4", F32)
                bias5 = load_const("bias5", F32)

                def conv_l3(zn, zT, bw_pref, bias_t, func, keep):
                    t1T = wpool.tile([128, 128], F16, tag=keep + "t1T", name=keep + "t1T")
                    ps = ppool.tile([128, 512], F32, tag="ps", name="ps")
                    nc.tensor.matmul(out=ps[:, :128], lhsT=zn, rhs=s3t[:], start=True, stop=True)
                    nc.scalar.activation(out=t1T[:], in_=ps[:, :128], func=AF.Copy)
                    t1n_ = wpool.tile([128, 128], F16, tag=keep + "t1n", name=keep + "t1n")
                    transp(t1T[:], t1n_[:])
                    p2T_ = wpool.tile([128, 128], F16, tag=keep + "p2T", name=keep + "p2T")
                    ps2 = ppool.tile([128, 512], F32, tag="ps", name="ps")
                    nc.tensor.matmul(out=ps2[:, :128], lhsT=t1n_[:], rhs=s3t[:], start=True, stop=True)
                    nc.scalar.activation(out=p2T_[:], in_=ps2[:, :128], func=AF.Copy)
                    bw = [load_const(f"{bw_pref}_{t}") for t in range(3)]
                    outT = wpool.tile([128, 128], F16, tag=keep + "oT", name=keep + "oT")
                    einsum_win(bw, [zT, t1T[:], p2T_[:]], 128, 128, outT[:], func, bias_t[:, 0:1])
                    outn = wpool.tile([128, 128], F16, tag=keep + "on", name=keep + "on")
                    transp(outT[:], outn[:])
                    return outn, outT

                z4n, z4T = conv_l3(z3n[:], z3T[:], "bigw3", bias4, AF.Tanh, "c4")
                o5n, o5T = conv_l3(z4n[:], z4T[:], "bigw4", bias5, AF.Copy, "c5")

            # ================= MLP input assembly =================
            with nc.named_scope("mlp_in"):
                nc.sync.dma_start(
                    out=x_loc.ap().rearrange("b (n c) -> n b c", c=32),
                    in_=o5n[:].rearrange("n (b c) -> n b c", c=32))
                ag(x_loc.ap(), x_all.ap())
                xT_sb = bigpool.tile([32, 4096], F16, tag="xT_sb", name="xT_sb")
                nc.sync.dma_start(out=xT_sb[:], in_=x_all[:, :])
                act6 = bigpool.tile([128, 1024], F16, tag="act6", name="act6")
                for i in range(32):
                    transp(xT_sb[:, 128 * i:128 * (i + 1)], act6[:, 32 * i:32 * i + 32])

            # ================= MLP =================
            def mlp_layer(li, act_sb, out_sb):
                g_t = load_const(f"g{li}", F32)
                be_t = load_const(f"be{li}", F32)
                wt = wlpool.tile([128, 32 * 512], F16, tag="wld", name="wld")
                nc.sync.dma_start(out=wt[:, :8192], in_=ein[f"w{li}"][:, :8192])
                nc.sync.dma_start(out=wt[:, 8192:], in_=ein[f"w{li}"][:, 8192:])
                acc = apool.tile([128, 512], F32, tag="acc", name="acc")
                for k2 in range(32):
                    nc.tensor.matmul(out=acc[:32, :], lhsT=act_sb[:, 32 * k2:32 * k2 + 32],
                                     rhs=wt[:, 512 * k2:512 * (k2 + 1)],
                                     start=(k2 == 0), stop=(k2 == 31))
                hb = wpool.tile([32, 512], F16, tag="hb", name="hb")
                nc.scalar.activation(out=hb[:], in_=acc[:32, :], func=AF.Copy)
                for c in range(4):
                    hc = wpool.tile([128, 32], F16, tag="hc", name="hc")
                    transp(hb[:, 128 * c:128 * (c + 1)], hc[:])
                    s1 = wpool.tile([128, 1], F32, tag="b_s1", name="b_s1")
                    nc.vector.tensor_reduce(out=s1[:], in_=hc[:], axis=AX.X, op=ALU.add)
                    mu_ = wpool.tile([128, 1], F32, tag="b_mu", name="b_mu")
                    nc.vector.tensor_scalar_mul(mu_[:], s1[:], 1.0 / 32.0)
                    sq = wpool.tile([128, 32], F32, tag="b_sq", name="b_sq")
                    nc.vector.tensor_mul(sq[:], hc[:], hc[:])
                    s2_ = wpool.tile([128, 1], F32, tag="b_s2", name="b_s2")
                    nc.vector.tensor_reduce(out=s2_[:], in_=sq[:], axis=AX.X, op=ALU.add)
                    var = wpool.tile([128, 1], F32, tag="b_var", name="b_var")
                    nc.vector.scalar_tensor_tensor(out=var[:], in0=mu_[:], scalar=-1.0,
                                                   in1=mu_[:], op0=ALU.mult, op1=ALU.mult)
                    nc.vector.scalar_tensor_tensor(out=var[:], in0=s2_[:], scalar=1.0 / 32.0,
                                                   in1=var[:], op0=ALU.mult, op1=ALU.add)
                    sd = wpool.tile([128, 1], F32, tag="b_sd", name="b_sd")
                    nc.scalar.activation(out=sd[:], in_=var[:], func=AF.Sqrt, bias=eps_t[:, 0:1])
                    rs = wpool.tile([128, 1], F32, tag="b_rs", name="b_rs")
                    nc.vector.reciprocal(rs[:], sd[:])
                    a_ = wpool.tile([128, 1], F32, tag="b_a", name="b_a")
                    nc.vector.tensor_mul(a_[:], rs[:], g_t[:, c:c + 1])
                    sh = wpool.tile([128, 1], F32, tag="b_sh", name="b_sh")
                    nc.vector.scalar_tensor_tensor(out=sh[:], in0=mu_[:], scalar=-1.0,
                                                   in1=a_[:], op0=ALU.mult, op1=ALU.mult)
                    nc.vector.tensor_add(sh[:], sh[:], be_t[:, c:c + 1])
                    nc.scalar.activation(out=out_sb[:, 32 * c:32 * c + 32], in_=hc[:],
                                         func=AF.Relu, scale=a_[:, 0:1], bias=sh[:, 0:1])

            with nc.named_scope("mlp6"):
                h6 = bigpool.tile([128, 128], F16, tag="h6sb", name="h6sb")
                mlp_layer(6, act6, h6)
                nc.sync.dma_start(out=h6_loc.ap(), in_=h6[:])
                ag(h6_loc.ap(), h6_all.ap())
            with nc.named_scope("mlp7"):
                act7 = bigpool.tile([128, 1024], F16, tag="act7", name="act7")
                for r in range(8):
                    nc.sync.dma_start(out=act7[:, 128 * r:128 * (r + 1)],
                                      in_=h6_all[128 * r:128 * (r + 1), :])
                h7 = bigpool.tile([128, 128], F16, tag="h7sb", name="h7sb")
                mlp_layer(7, act7, h7)
                nc.sync.dma_start(out=h7_loc.ap(), in_=h7[:])
                ag(h7_loc.ap(), h7_all.ap())
            with nc.named_scope("mlp8"):
                act8 = bigpool.tile([128, 1024], F16, tag="act8", name="act8")
                for r in range(8):
                    nc.sync.dma_start(out=act8[:, 128 * r:128 * (r + 1)],
                                      in_=h7_all[128 * r:128 * (r + 1), :])
                h8 = bigpool.tile([128, 128], F16, tag="h8sb", name="h8sb")
                mlp_layer(8, act8, h8)

            with nc.named_scope("mlp9"):
                w9t = cpool.tile([128, 512], F16, tag="w9t", name="w9t")
                nc.sync.dma_start(out=w9t[:], in_=ein["w9"][:, :])
                acc9 = apool.tile([128, 512], F32, tag="acc", name="acc9")
                for c in range(4):
                    nc.tensor.matmul(out=acc9[:32, :128], lhsT=h8[:, 32 * c:32 * c + 32],
                                     rhs=w9t[:, 128 * c:128 * (c + 1)],
                                     start=(c == 0), stop=(c == 3))
                p9sb = wpool.tile([32, 128], F32, tag="p9sb", name="p9sb")
                nc.scalar.activation(out=p9sb[:], in_=acc9[:32, :128], func=AF.Copy)
                nc.sync.dma_start(out=p9_loc.ap(), in_=p9sb[:])
                ag(p9_loc.ap(), p9_all.ap())
                tot = wpool.tile([32, 128], F32, tag="f_tot", name="f_tot")
                nc.sync.dma_start(out=tot[:], in_=p9_all[0:32, :])
                for k in range(1, 8):
                    pk = wpool.tile([32, 128], F32, tag="f_pk", name="f_pk")
                    nc.sync.dma_start(out=pk[:], in_=p9_all[32 * k:32 * (k + 1), :])
                    nc.vector.tensor_add(tot[:], tot[:], pk[:])
                totT = wpool.tile([128, 32], F32, tag="f_totT", name="f_totT")
                pst = ppool.tile([128, 512], F32, tag="ps", name="pst")
                identf = cpool.tile([32, 32], F32, tag="identf", name="identf")
                nc.scalar.activation(out=identf[:], in_=ident[:32, :32], func=AF.Copy)
                nc.tensor.transpose(out=pst[:128, :32], in_=tot[:], identity=identf[:])
                nc.scalar.activation(out=totT[:], in_=pst[:128, :32], func=AF.Copy)
                s1 = wpool.tile([128, 1], F32, tag="f_s1", name="f_s1")
                nc.vector.tensor_reduce(out=s1[:], in_=totT[:], axis=AX.X, op=ALU.add)
                mu_ = wpool.tile([128, 1], F32, tag="f_mu", name="f_mu")
                nc.vector.tensor_scalar_mul(mu_[:], s1[:], 1.0 / 32.0)
                sq = wpool.tile([128, 32], F32, tag="f_sq", name="f_sq")
                nc.vector.tensor_mul(sq[:], totT[:], totT[:])
                s2_ = wpool.tile([128, 1], F32, tag="f_s2", name="f_s2")
                nc.vector.tensor_reduce(out=s2_[:], in_=sq[:], axis=AX.X, op=ALU.add)
                var = wpool.tile([128, 1], F32, tag="f_var", name="f_var")
                nc.vector.scalar_tensor_tensor(out=var[:], in0=mu_[:], scalar=-1.0,
                                               in1=mu_[:], op0=ALU.mult, op1=ALU.mult)
                nc.vector.scalar_tensor_tensor(out=var[:], in0=s2_[:], scalar=1.0 / 32.0,
                                               in1=var[:], op0=ALU.mult, op1=ALU.add)
                sdf = wpool.tile([128, 1], F32, tag="f_sd", name="f_sd")
                nc.scalar.activation(out=sdf[:], in_=var[:], func=AF.Sqrt, bias=eps_t[:, 0:1])
                rs = wpool.tile([128, 1], F32, tag="f_rs", name="f_rs")
                nc.vector.reciprocal(rs[:], sdf[:])
                neg = wpool.tile([128, 1], F32, tag="f_neg", name="f_neg")
                nc.vector.scalar_tensor_tensor(out=neg[:], in0=mu_[:], scalar=-1.0,
                                               in1=rs[:], op0=ALU.mult, op1=ALU.mult)
                outt = wpool.tile([128, 32], F32, tag="f_out", name="f_out")
                nc.scalar.activation(out=outt[:], in_=totT[:], func=AF.Identity,
                                     scale=rs[:, 0:1], bias=neg[:, 0:1])
                nc.sync.dma_start(out=out_mu[:, :], in_=outt[:])

    nc.compile()
    return nc


# ---------------------------------------------------------------- entry point
def kernel(**inputs) -> np.ndarray:
    per_core, meta = _host_prep(inputs)
    if "prog" not in _CACHE:
        _CACHE["prog"] = _build_nc(meta, per_core[0])
    nc = _CACHE["prog"]
    res = bass_utils.run_bass_kernel_spmd(nc, per_core, core_ids=list(range(NCORES)))
    return np.ascontiguousarray(res.results[0]["mu"].T)


# revision 30
# speedup vs baseline: 3.0293x; 1.0166x over previous
"""Trainium2 Bass kernel for nn_Encoder_base (5x ChebConv GNN + pool + MLP).

Distribution over 8 NeuronCores (all matmuls fp16, fp32 PSUM):
  - level 0: the two props the einsum needs (Tx1[l0] = S0[l0]@X0 and
    p2t = S0[l0]@S0@X0) are composed on the HOST into single operators on
    the input X0 (2-hop edge expansion M0 = S0[l0]*S0). Edge-major X0 rows
    are pregathered host-side -> the props are pure streaming selection
    matmuls: zero indirect DMA, zero full-graph AllGather.
  - level 1: stacked dense operator T = [P_l1; S1[l1]; M1=S1[l1]*S1]
    (3072 x 4096) applied to z1, dest-sharded (128 l1-positions/core, all
    1024 batch-features wide); einsum is dest-sharded too. Comm: one
    chunked z1 AllGather + one small z2 AllGather.
  - levels 2-3: batch-sharded (4 batches/core), dense-S matmuls,
    block-diagonal channel mixes in feature-major layout.
  - MLP: output-feature sharded (512 cols of W6/7/8, 512 rows of W9 per
    core); activations [128k,32] are the stationary lhsT, W streams as rhs;
    BatchNorm per-feature after a PE transpose; activations AllGathered.
"""
import numpy as np
import concourse.bass as bass
import concourse.bacc as bacc
import concourse.tile as tile
from concourse import mybir, bass_utils

F32 = mybir.dt.float32
F16 = mybir.dt.float16
I16 = mybir.dt.int16
AF = mybir.ActivationFunctionType
ALU = mybir.AluOpType
AX = mybir.AxisListType
RG = [list(range(8))]
NCORES = 8
N0, N1, N2, N3 = 16384, 4096, 1024, 128
EPS = 1e-5
H16 = np.float16

_CACHE = {}


# ---------------------------------------------------------------- host prep
def _prep_prop(row, col, we, n_dest, n_shard):
    """Sorted-by-dest edges -> 128-dest windows, 128-edge chunks, padded so
    chunk counts per window match across shards (one SPMD program).
    Emits per-chunk selection matrices sel[chunk, edge_local, dst_local]."""
    window = 128
    order = np.argsort(row, kind="stable")
    row, col, we = row[order], col[order], we[order]
    per = n_dest // n_shard
    nwin = per // window
    counts = np.zeros((n_shard, nwin), np.int64)
    lists = {}
    for s in range(n_shard):
        lo = s * per
        for wi in range(nwin):
            wlo = lo + wi * window
            a = np.searchsorted(row, wlo, side="left")
            b = np.searchsorted(row, wlo + window, side="left")
            lists[(s, wi)] = (row[a:b] - wlo, col[a:b], we[a:b])
            counts[s, wi] = (b - a + 127) // 128
    ncw = np.maximum(counts.max(axis=0), 1)
    C = int(ncw.sum())
    src = np.zeros((n_shard, C, 128), np.int64)
    sel = np.zeros((n_shard, C, 128, 128), np.float32)
    for s in range(n_shard):
        base = 0
        for wi in range(nwin):
            dl, cl, wl = lists[(s, wi)]
            n = len(dl)
            k = int(ncw[wi])
            src[s, base:base + k].reshape(-1)[:n] = cl
            ch = base + np.arange(n) // 128
            ep = np.arange(n) % 128
            sel[s, ch, ep, dl] = wl
            base += k
    return [int(x) for x in ncw], src, sel


def _edge_we(e, n):
    row, col = np.asarray(e[0], np.int64), np.asarray(e[1], np.int64)
    deg = np.bincount(row, minlength=n).astype(np.float32)
    dis = np.where(deg > 0, 1.0 / np.sqrt(np.maximum(deg, 1.0)), 0.0).astype(np.float32)
    return row, col, -(dis[row] * dis[col]).astype(np.float32)


def _sub_edges(row, col, we, pool_idx):
    order = np.argsort(row, kind="stable")
    row, col, we = row[order], col[order], we[order]
    starts = np.searchsorted(row, pool_idx, side="left")
    ends = np.searchsorted(row, pool_idx, side="right")
    nr, ncl, nw = [], [], []
    for i in range(len(pool_idx)):
        s, e = starts[i], ends[i]
        if e > s:
            nr.append(np.full(e - s, i, np.int64))
            ncl.append(col[s:e])
            nw.append(we[s:e])
    return np.concatenate(nr), np.concatenate(ncl), np.concatenate(nw)


def _twohop(ri, ci, wi, row, col, we, n):
    """(i,j,w1) sub-edges composed with full edges (j->k,w2): (i,k,w1*w2)."""
    order = np.argsort(row, kind="stable")
    rs, cs, ws = row[order], col[order], we[order]
    starts = np.searchsorted(rs, np.arange(n), side="left")
    ends = np.searchsorted(rs, np.arange(n), side="right")
    cnt = (ends - starts)[ci]
    I = np.repeat(ri, cnt)
    W1 = np.repeat(wi, cnt)
    base = np.repeat(starts[ci], cnt)
    within = np.arange(cnt.sum()) - np.repeat(np.cumsum(cnt) - cnt, cnt)
    offs = base + within
    return I, cs[offs], W1 * ws[offs]


def _dense_s(row, col, we, n, m):
    s = np.zeros((n, m), np.float32)
    np.add.at(s, (row, col), we)
    return s


def _tile_w(w, pack):
    """[K, M] -> [K//(128*pack) * 128, pack*M]: pack K-blocks side by side."""
    k, m = w.shape
    nb = k // 128
    t = w.reshape(nb // pack, pack, 128, m).transpose(0, 2, 1, 3)
    return np.ascontiguousarray(t.reshape((nb // pack) * 128, pack * m))


def _idx_tile(flat):
    """flat int idx list -> [128, len//16] int16 (16-part wrap, x8 replicas)."""
    return np.ascontiguousarray(
        np.tile(flat.astype(np.int16).reshape(-1, 16).T, (8, 1)))


def _chunk_tile(arr3):
    """[C, 128, W] -> [128, C*W] (chunk c at cols c*W..)."""
    C, _, W = arr3.shape
    return np.ascontiguousarray(
        arr3.transpose(1, 0, 2).reshape(128, C * W)).astype(H16)


def _host_prep(inputs):
    d = {k: np.asarray(v) for k, v in inputs.items()}
    x = d["x"].astype(np.float32)
    l0 = np.asarray(d["l0"], np.int64)
    l1 = np.asarray(d["l1"], np.int64)
    l2 = np.asarray(d["l2"], np.int64)

    X0 = np.ascontiguousarray(x.transpose(1, 0, 2).reshape(N0, 96))
    X0p = np.zeros((N0, 128), np.float32)
    X0p[:, :96] = X0
    X0l0T = np.ascontiguousarray(X0[l0].T)  # [96, 4096]

    # level-0 operators on X0: a = S0[l0] (tap1), m = S0[l0]@S0 (tap2)
    r0, c0, w0 = _edge_we(d["e0"], N0)
    r0s, c0s, w0s = _sub_edges(r0, c0, w0, l0)
    ncw_a, src_a, sel_a = _prep_prop(r0s, c0s, w0s, N1, NCORES)
    mI, mK, mW = _twohop(r0s, c0s, w0s, r0, c0, w0, N0)
    ncw_m, src_m, sel_m = _prep_prop(mI, mK, mW, N1, NCORES)

    # level-1 stacked operator T = [P_l1; S1[l1]; M1]
    r1, c1, w1 = _edge_we(d["e1"], N1)
    S1 = _dense_s(r1, c1, w1, N1, N1)
    r1s, c1s, w1s = _sub_edges(r1, c1, w1, l1)
    S1l1 = _dense_s(r1s, c1s, w1s, N2, N1)    # [1024, 4096]
    M1 = S1l1 @ S1                            # [1024, 4096]
    P_l1 = np.zeros((N2, N1), np.float32)
    P_l1[np.arange(N2), l1] = 1.0
    Tblocks = [P_l1, S1l1, M1]

    r2, c2, w2 = _edge_we(d["e2"], N2)
    S2 = _dense_s(r2, c2, w2, N2, N2)
    S2T = _tile_w(np.ascontiguousarray(S2.T), 8).astype(H16)       # [128, 8192]
    S2l2T = _tile_w(np.ascontiguousarray(S2[l2].T), 8).astype(H16)  # [128, 1024]
    P_l2 = np.zeros((N2, 128), np.float32)
    P_l2[l2, np.arange(128)] = 1.0
    P_l2 = _tile_w(P_l2, 8).astype(H16)                             # [128, 1024]

    r3, c3, w3 = _edge_we(d["e3"], N3)
    S3T = np.ascontiguousarray(_dense_s(r3, c3, w3, N3, N3).T).astype(H16)

    def wmod(W):
        return W[0] - W[2], W[1], 2.0 * W[2]

    Wm1 = wmod(d["Wc1"].astype(np.float32))
    Wm = [wmod(d[f"Wc{i}"].astype(np.float32)) for i in (2, 3, 4, 5)]
    eye4 = np.eye(4, dtype=np.float32)

    per_core = []
    for k in range(NCORES):
        m = {}
        m["identbf"] = np.eye(128, dtype=np.float32).astype(H16)
        m["epsv"] = np.full((128, 1), EPS, np.float32)
        m["warm"] = np.zeros((1, 8), np.float32)
        m["X0l0Tw"] = np.ascontiguousarray(
            X0l0T[:, 512 * k:512 * (k + 1)]).astype(H16)
        m["Xg_a"] = _chunk_tile(X0p[src_a[k]].astype(H16))
        m["sel_a"] = _chunk_tile(sel_a[k])
        m["Xg_m"] = _chunk_tile(X0p[src_m[k]].astype(H16))
        m["sel_m"] = _chunk_tile(sel_m[k])
        # stacked-T lhsT chunks: block b, k-chunk kk at cols (b*32+kk)*128
        tt = np.zeros((128, 96 * 128), np.float32)
        for b, blk in enumerate(Tblocks):
            bt = blk[128 * k:128 * (k + 1), :].T  # [4096, 128]
            for kk in range(32):
                tt[:, (b * 32 + kk) * 128:(b * 32 + kk + 1) * 128] = \
                    bt[128 * kk:128 * (kk + 1), :]
        m["Tt"] = tt.astype(H16)
        # z2n extraction: for node-chunk c, rows 1024c + 128k + j of z2T_all
        m["z2n_idx"] = _idx_tile(np.concatenate(
            [1024 * c + 128 * k + np.arange(128) for c in range(8)]))
        m["S2T"] = S2T
        m["S2l2T"] = S2l2T
        m["P_l2"] = P_l2
        m["S3T"] = S3T
        for g in range(8):
            for t in range(3):
                bw = np.zeros((96, 128), np.float32)
                for j in range(4):
                    bg = 4 * g + j
                    bw[3 * bg:3 * bg + 3, 32 * j:32 * j + 32] = Wm1[t]
                m[f"bigw0_{g}_{t}"] = bw.astype(H16)
        for lev in range(4):
            for t in range(3):
                m[f"bigw{lev + 1}_{t}"] = np.kron(eye4, Wm[lev][t]).astype(H16)
        for lev, nm in ((1, "b1"), (2, "b2"), (3, "b3"), (4, "b4"), (5, "b5")):
            m[f"bias{lev}"] = np.tile(d[nm].astype(np.float32), 4).reshape(128, 1)
        for li in (6, 7, 8):
            W = d[f"W{li}"].astype(np.float32)[:, 512 * k:512 * k + 512]
            m[f"w{li}"] = np.ascontiguousarray(
                W.reshape(32, 128, 512).transpose(1, 0, 2).reshape(128, 32 * 512)
            ).astype(H16)
            m[f"g{li}"] = np.ascontiguousarray(
                d[f"g{li}"].astype(np.float32)[512 * k:512 * k + 512].reshape(4, 128).T)
            m[f"be{li}"] = np.ascontiguousarray(
                d[f"be{li}"].astype(np.float32)[512 * k:512 * k + 512].reshape(4, 128).T)
        W9 = d["W9"].astype(np.float32)[512 * k:512 * k + 512]  # [512, 128]
        m["w9"] = np.ascontiguousarray(
            W9.reshape(4, 128, 128).transpose(1, 0, 2).reshape(128, 512)).astype(H16)
        per_core.append(m)

    meta = {"a": ncw_a, "m": ncw_m}
    return per_core, meta


# ---------------------------------------------------------------- device program
def _build_nc(meta, shapes):
    nc = bacc.Bacc("TRN2", target_bir_lowering=False, debug=False, num_devices=NCORES)
    ein = {}
    for name, arr in shapes.items():
        dt = {np.dtype(np.int16): I16, np.dtype(H16): F16,
              np.dtype(np.float32): F32}[arr.dtype]
        ein[name] = nc.dram_tensor(name, list(arr.shape), dt, kind="ExternalInput")
    out_mu = nc.dram_tensor("mu", [128, 32], F32, kind="ExternalOutput")

    warm_all = nc.dram_tensor("warm_all", [8, 8], F32)
    warm_loc = nc.dram_tensor("warm_loc", [1, 8], F32)
    z1c = [nc.dram_tensor(f"z1c_{i}", [128, 1024], F16) for i in range(4)]
    z1ag = [nc.dram_tensor(f"z1ag_{i}", [1024, 1024], F16) for i in range(4)]
    z2T_loc = nc.dram_tensor("z2T_loc", [1024, 128], F16)
    z2T_all = nc.dram_tensor("z2T_all", [8192, 128], F16)
    x_loc = nc.dram_tensor("x_loc", [4, 4096], F16)
    x_all = nc.dram_tensor("x_all", [32, 4096], F16)
    h6_loc = nc.dram_tensor("h6_loc", [128, 128], F16)
    h6_all = nc.dram_tensor("h6_all", [1024, 128], F16)
    h7_loc = nc.dram_tensor("h7_loc", [128, 128], F16)
    h7_all = nc.dram_tensor("h7_all", [1024, 128], F16)
    p9_loc = nc.dram_tensor("p9_loc", [32, 128], F32)
    p9_all = nc.dram_tensor("p9_all", [256, 128], F32)

    def ag(loc_ap, all_ap):
        nc.gpsimd.collective_compute(
            "AllGather", ALU.bypass, replica_groups=RG,
            ins=[loc_ap.opt()], outs=[all_ap.opt()])

    with tile.TileContext(nc) as tc:
        with (
            tc.tile_pool(name="const", bufs=1) as cpool,
            tc.tile_pool(name="big", bufs=1) as bigpool,
            tc.tile_pool(name="work", bufs=3) as wpool,
            tc.tile_pool(name="wload", bufs=2) as wlpool,
            tc.tile_pool(name="psA", bufs=3, space="PSUM") as ppool,
            tc.tile_pool(name="psT", bufs=2, space="PSUM") as tpool,
            tc.tile_pool(name="psB", bufs=1, space="PSUM") as apool,
        ):
            ident = cpool.tile([128, 128], F16, tag="identbf", name="identbf")
            nc.sync.dma_start(out=ident[:], in_=ein["identbf"][:, :])
            eps_t = cpool.tile([128, 1], F32, tag="epsv", name="epsv")
            nc.sync.dma_start(out=eps_t[:], in_=ein["epsv"][:, :])

            def load_const(name, dt=F16):
                t = cpool.tile(list(shapes[name].shape), dt, tag=name)
                nc.sync.dma_start(out=t[:], in_=ein[name][:, :])
                return t

            GRP = 16

            def grp_load(pref, g0, gc, tag, eng=None, grp=None):
                sl = wpool.tile([128, (grp or GRP) * 128], F16, tag=tag,
                                name=tag, bufs=2)
                (eng or nc.sync).dma_start(out=sl[:, :gc * 128],
                                           in_=ein[pref][:, g0 * 128:(g0 + gc) * 128])
                return sl

            def transp(src_ap, dst_ap):
                p, f = src_ap.shape
                ps = tpool.tile([128, 128], F16, tag="tp", name="tp")
                nc.tensor.transpose(out=ps[:f, :p], in_=src_ap, identity=ident[:p, :p])
                nc.scalar.activation(out=dst_ap, in_=ps[:f, :p], func=AF.Copy)

            def einsum_win(bigw, taps, Din, width, out_ap, func, bias_ap):
                ps = ppool.tile([128, 512], F32, tag="ps", name="ps")
                for t in range(3):
                    nc.tensor.matmul(out=ps[:, :width], lhsT=bigw[t][:Din, :],
                                     rhs=taps[t], start=(t == 0), stop=(t == 2))
                f2 = AF.Identity if func == AF.Copy else func
                nc.scalar.activation(out=out_ap, in_=ps[:, :width], func=f2, bias=bias_ap)

            # warm up the CC ring while level-0 computes
            with nc.named_scope("warmup"):
                warm = wpool.tile([1, 8], F32, tag="warm", name="warm")
                nc.sync.dma_start(out=warm[:], in_=ein["warm"][:, :])
                nc.sync.dma_start(out=warm_loc.ap(), in_=warm[:])
                ag(warm_loc.ap(), warm_all.ap())

            # ====== LEVEL 0: per-window pipeline of props -> einsum -> AG ====
            # tap1 = S0[l0]@X0, tap2 = (S0[l0]@S0)@X0, then the channel-mix
            # einsum for window wi immediately, then AllGather that window.
            with nc.named_scope("l0"):
                Ca, Cm = sum(meta["a"]), sum(meta["m"])
                cura = {"g0": -1}
                curm = {"g0": -1}

                def get_grp(cur, cc, C, xg, sel, xtag, stag, grp):
                    g0 = (cc // grp) * grp
                    if g0 != cur["g0"]:
                        gc = min(grp, C - g0)
                        cur["g0"] = g0
                        cur["x"] = grp_load(xg, g0, gc, xtag, eng=nc.scalar, grp=grp)
                        cur["s"] = grp_load(sel, g0, gc, stag, grp=grp)
                    return cur, (cc - cur["g0"]) * 128

                bias1 = load_const("bias1", F32)
                x0w = cpool.tile([96, 512], F16, tag="X0l0Tw", name="X0l0Tw")
                nc.sync.dma_start(out=x0w[:], in_=ein["X0l0Tw"][:, :])
                bw0g = [[load_const(f"bigw0_{g}_{t}") for t in range(3)]
                        for g in range(8)]
                tap1_sb = bigpool.tile([96, 512], F16, tag="tap1_sb", name="tap1_sb")
                tap2_sb = bigpool.tile([96, 512], F16, tag="tap2_sb", name="tap2_sb")
                ba, bm = 0, 0
                for wi in range(4):
                    psa = ppool.tile([128, 512], F32, tag="ps", name="ps")
                    for c in range(meta["a"][wi]):
                        g, lo = get_grp(cura, ba + c, Ca, "Xg_a", "sel_a", "xga", "sla", 8)
                        nc.tensor.matmul(out=psa[:96, :128], lhsT=g["x"][:, lo:lo + 96],
                                         rhs=g["s"][:, lo:lo + 128],
                                         start=(c == 0), stop=(c == meta["a"][wi] - 1))
                    nc.scalar.activation(out=tap1_sb[:, wi * 128:(wi + 1) * 128],
                                         in_=psa[:96, :128], func=AF.Copy)
                    ba += meta["a"][wi]
                    psm = ppool.tile([128, 512], F32, tag="ps", name="ps")
                    for c in range(meta["m"][wi]):
                        g, lo = get_grp(curm, bm + c, Cm, "Xg_m", "sel_m", "xgm", "slm", 16)
                        nc.tensor.matmul(out=psm[:96, :128], lhsT=g["x"][:, lo:lo + 96],
                                         rhs=g["s"][:, lo:lo + 128],
                                         start=(c == 0), stop=(c == meta["m"][wi] - 1))
                    nc.scalar.activation(out=tap2_sb[:, wi * 128:(wi + 1) * 128],
                                         in_=psm[:96, :128], func=AF.Copy)
                    bm += meta["m"][wi]
                    for g in range(8):
                        z1gT = wpool.tile([128, 128], F16, tag="z1Tw", name="z1Tw")
                        einsum_win(bw0g[g],
                                   [x0w[:, wi * 128:(wi + 1) * 128],
                                    tap1_sb[:96, wi * 128:(wi + 1) * 128],
                                    tap2_sb[:96, wi * 128:(wi + 1) * 128]],
                                   96, 128, z1gT[:], AF.Copy, bias1[:, 0:1])
                        t = wpool.tile([128, 128], F16, tag="z1nc", name="z1nc")
                        transp(z1gT[:], t[:])
                        nc.sync.dma_start(
                            out=z1c[wi][:, 128 * g:128 * (g + 1)], in_=t[:])
                    with nc.named_scope(f"agz1_{wi}"):
                        ag(z1c[wi].ap(), z1ag[wi].ap())

            # ====== LEVEL 1: taps = T @ z1, dest-sharded (128 pos x 1024) ====
            with nc.named_scope("l1_T"):
                tt = cpool.tile([128, 96 * 128], F16, tag="Tt", name="Tt")
                nc.sync.dma_start(out=tt[:, :6144], in_=ein["Tt"][:, :6144])
                nc.sync.dma_start(out=tt[:, 6144:], in_=ein["Tt"][:, 6144:])
                accs = [bigpool.tile([128, 1024], F16, tag=f"accT{b}", name=f"accT{b}")
                        for b in range(3)]
                for part in range(4):
                    for h in range(2):
                        z1sc = wpool.tile([128, 4096], F16, tag="z1sc", name="z1sc", bufs=2)
                        nc.sync.dma_start(
                            out=z1sc[:].rearrange("p (q d) -> p q d", d=1024),
                            in_=z1ag[part].ap()[512 * h:512 * (h + 1)]
                                .rearrange("(q p) d -> p q d", p=128))
                        for b in range(3):
                            psa = ppool.tile([128, 512], F32, tag="ps", name="ps")
                            psb = ppool.tile([128, 512], F32, tag="ps", name="ps")
                            for q in range(4):
                                kk = 4 * (4 * h + q) + part
                                lh = tt[:, (b * 32 + kk) * 128:(b * 32 + kk + 1) * 128]
                                nc.tensor.matmul(out=psa[:, :512], lhsT=lh,
                                                 rhs=z1sc[:, q * 1024:q * 1024 + 512],
                                                 start=(q == 0), stop=(q == 3))
                                nc.tensor.matmul(out=psb[:, :512], lhsT=lh,
                                                 rhs=z1sc[:, q * 1024 + 512:(q + 1) * 1024],
                                                 start=(q == 0), stop=(q == 3))
                            if part == 0 and h == 0:
                                nc.scalar.activation(out=accs[b][:, :512], in_=psa[:, :512],
                                                     func=AF.Copy)
                                nc.scalar.activation(out=accs[b][:, 512:], in_=psb[:, :512],
                                                     func=AF.Copy)
                            else:
                                nc.vector.tensor_add(accs[b][:, :512], accs[b][:, :512],
                                                     psa[:, :512])
                                nc.vector.tensor_add(accs[b][:, 512:], accs[b][:, 512:],
                                                     psb[:, :512])

            # ============ LEVEL 1 einsum (dest-sharded) -> z2T ===============
            with nc.named_scope("l1_einsum"):
                bw1 = [load_const(f"bigw1_{t}") for t in range(3)]
                bias2 = load_const("bias2", F32)
                tapTs = []
                for b in range(3):
                    tapT = bigpool.tile([128, 1024], F16, tag=f"tapT{b}", name=f"tapT{b}")
                    for f in range(8):
                        transp(accs[b][:, 128 * f:128 * (f + 1)],
                               tapT[:, 128 * f:128 * (f + 1)])
                    tapTs.append(tapT)
                for fg in range(8):
                    z2fg = wpool.tile([128, 128], F16, tag="z2fg", name="z2fg")
                    einsum_win(bw1, [tapTs[0][:, 128 * fg:128 * (fg + 1)],
                                     tapTs[1][:, 128 * fg:128 * (fg + 1)],
                                     tapTs[2][:, 128 * fg:128 * (fg + 1)]],
                               128, 128, z2fg[:], AF.Tanh, bias2[:, 0:1])
                    nc.sync.dma_start(out=z2T_loc[128 * fg:128 * (fg + 1), :],
                                      in_=z2fg[:])
            with nc.named_scope("ag_z2"):
                ag(z2T_loc.ap(), z2T_all.ap())

            # ====== z2n assembly (batch-sharded node-major) ==================
            z2n = bigpool.tile([128, 8 * 128], F16, tag="z2n", name="z2n")
            with nc.named_scope("z2n_asm"):
                z2i = cpool.tile([128, 8 * 8], I16, tag="z2i", name="z2i")
                nc.sync.dma_start(out=z2i[:], in_=ein["z2n_idx"][:, :])
                zb = wpool.tile([128, 1024], F16, tag="zb", name="zb", bufs=1)
                nc.gpsimd.dma_gather(
                    out_ap=zb[:].rearrange("p (c e) -> p c e", e=128),
                    in_ap=z2T_all[:, :],
                    idxs_ap=z2i[:],
                    num_idxs=1024, num_idxs_reg=1024, elem_size=128,
                    single_packet=False)
                for ci in range(8):
                    transp(zb[:, ci * 128:(ci + 1) * 128],
                           z2n[:, ci * 128:(ci + 1) * 128])

            # ================= LEVEL 2 (dense) =================
            with nc.named_scope("l2"):
                t1_l2 = bigpool.tile([128, 8 * 128], F16, tag="t1_l2", name="t1_l2")
                for half in range(2):
                    s2t = wlpool.tile([128, 4096], F16, tag="wld", name="wld")
                    nc.sync.dma_start(out=s2t[:], in_=ein["S2T"][:, 4096 * half:4096 * (half + 1)])
                    for dc in range(8):
                        ps = ppool.tile([128, 512], F32, tag="ps", name="ps")
                        for kk in range(4):
                            kc = half * 4 + kk
                            nc.tensor.matmul(
                                out=ps[:, :128],
                                lhsT=s2t[:, kk * 1024 + dc * 128: kk * 1024 + dc * 128 + 128],
                                rhs=z2n[:, kc * 128:(kc + 1) * 128],
                                start=(kk == 0), stop=(kk == 3))
                        if half == 0:
                            nc.scalar.activation(out=t1_l2[:, dc * 128:(dc + 1) * 128],
                                                 in_=ps[:, :128], func=AF.Copy)
                        else:
                            nc.vector.tensor_add(t1_l2[:, dc * 128:(dc + 1) * 128],
                                                 t1_l2[:, dc * 128:(dc + 1) * 128],
                                                 ps[:, :128])
                s2l2 = cpool.tile([128, 1024], F16, tag="s2l2", name="s2l2")
                nc.sync.dma_start(out=s2l2[:], in_=ein["S2l2T"][:, :])
                ps = ppool.tile([128, 512], F32, tag="ps", name="ps")
                for kc in range(8):
                    nc.tensor.matmul(out=ps[:, :128], lhsT=s2l2[:, kc * 128:(kc + 1) * 128],
                                     rhs=t1_l2[:, kc * 128:(kc + 1) * 128],
                                     start=(kc == 0), stop=(kc == 7))
                p2n_l2 = wpool.tile([128, 128], F16, tag="p2n_l2", name="p2n_l2")
                nc.scalar.activation(out=p2n_l2[:], in_=ps[:, :128], func=AF.Copy)
                pl2 = cpool.tile([128, 1024], F16, tag="pl2", name="pl2")
                nc.sync.dma_start(out=pl2[:], in_=ein["P_l2"][:, :])
                z2l2T = wpool.tile([128, 128], F16, tag="z2l2T", name="z2l2T")
                psg = ppool.tile([128, 512], F32, tag="ps", name="ps")
                for kc in range(8):
                    nc.tensor.matmul(out=psg[:, :128], lhsT=z2n[:, kc * 128:(kc + 1) * 128],
                                     rhs=pl2[:, kc * 128:(kc + 1) * 128],
                                     start=(kc == 0), stop=(kc == 7))
                nc.scalar.activation(out=z2l2T[:], in_=psg[:, :128], func=AF.Copy)
                t1l2T = wpool.tile([128, 128], F16, tag="t1l2T", name="t1l2T")
                psg2 = ppool.tile([128, 512], F32, tag="ps", name="ps")
                for kc in range(8):
                    nc.tensor.matmul(out=psg2[:, :128], lhsT=t1_l2[:, kc * 128:(kc + 1) * 128],
                                     rhs=pl2[:, kc * 128:(kc + 1) * 128],
                                     start=(kc == 0), stop=(kc == 7))
                nc.scalar.activation(out=t1l2T[:], in_=psg2[:, :128], func=AF.Copy)
                p2l2T = wpool.tile([128, 128], F16, tag="p2l2T", name="p2l2T")
                transp(p2n_l2[:], p2l2T[:])
                bw2 = [load_const(f"bigw2_{t}") for t in range(3)]
                bias3 = load_const("bias3", F32)
                z3T = wpool.tile([128, 128], F16, tag="z3T", name="z3T")
                einsum_win(bw2, [z2l2T[:], t1l2T[:], p2l2T[:]], 128, 128,
                           z3T[:], AF.Tanh, bias3[:, 0:1])
                z3n = wpool.tile([128, 128], F16, tag="z3n", name="z3n")
                transp(z3T[:], z3n[:])

            # ================= LEVEL 3 =================
            with nc.named_scope("l3"):
                s3t = cpool.tile([128, 128], F16, tag="s3t", name="s3t")
                nc.sync.dma_start(out=s3t[:], in_=ein["S3T"][:, :])
                bias4 = load_const("bias4", F32)
                bias5 = load_const("bias5", F32)

                def conv_l3(zn, zT, bw_pref, bias_t, func, keep):
                    t1T = wpool.tile([128, 128], F16, tag=keep + "t1T", name=keep + "t1T")
                    ps = ppool.tile([128, 512], F32, tag="ps", name="ps")
                    nc.tensor.matmul(out=ps[:, :128], lhsT=zn, rhs=s3t[:], start=True, stop=True)
                    nc.scalar.activation(out=t1T[:], in_=ps[:, :128], func=AF.Copy)
                    t1n_ = wpool.tile([128, 128], F16, tag=keep + "t1n", name=keep + "t1n")
                    transp(t1T[:], t1n_[:])
                    p2T_ = wpool.tile([128, 128], F16, tag=keep + "p2T", name=keep + "p2T")
                    ps2 = ppool.tile([128, 512], F32, tag="ps", name="ps")
                    nc.tensor.matmul(out=ps2[:, :128], lhsT=t1n_[:], rhs=s3t[:], start=True, stop=True)
                    nc.scalar.activation(out=p2T_[:], in_=ps2[:, :128], func=AF.Copy)
                    bw = [load_const(f"{bw_pref}_{t}") for t in range(3)]
                    outT = wpool.tile([128, 128], F16, tag=keep + "oT", name=keep + "oT")
                    einsum_win(bw, [zT, t1T[:], p2T_[:]], 128, 128, outT[:], func, bias_t[:, 0:1])
                    outn = wpool.tile([128, 128], F16, tag=keep + "on", name=keep + "on")
                    transp(outT[:], outn[:])
                    return outn, outT

                z4n, z4T = conv_l3(z3n[:], z3T[:], "bigw3", bias4, AF.Tanh, "c4")
                o5n, o5T = conv_l3(z4n[:], z4T[:], "bigw4", bias5, AF.Copy, "c5")

            # ================= MLP input assembly =================
            with nc.named_scope("mlp_in"):
                nc.sync.dma_start(
                    out=x_loc.ap().rearrange("b (n c) -> n b c", c=32),
                    in_=o5n[:].rearrange("n (b c) -> n b c", c=32))
                ag(x_loc.ap(), x_all.ap())
                xT_sb = bigpool.tile([32, 4096], F16, tag="xT_sb", name="xT_sb")
                nc.sync.dma_start(out=xT_sb[:], in_=x_all[:, :])
                act6 = bigpool.tile([128, 1024], F16, tag="act6", name="act6")
                for i in range(32):
                    transp(xT_sb[:, 128 * i:128 * (i + 1)], act6[:, 32 * i:32 * i + 32])

            # ================= MLP =================
            def mlp_layer(li, act_sb, out_sb):
                g_t = load_const(f"g{li}", F32)
                be_t = load_const(f"be{li}", F32)
                wt = wlpool.tile([128, 32 * 512], F16, tag="wld", name="wld")
                nc.sync.dma_start(out=wt[:, :8192], in_=ein[f"w{li}"][:, :8192])
                nc.sync.dma_start(out=wt[:, 8192:], in_=ein[f"w{li}"][:, 8192:])
                acc = apool.tile([128, 512], F32, tag="acc", name="acc")
                for k2 in range(32):
                    nc.tensor.matmul(out=acc[:32, :], lhsT=act_sb[:, 32 * k2:32 * k2 + 32],
                                     rhs=wt[:, 512 * k2:512 * (k2 + 1)],
                                     start=(k2 == 0), stop=(k2 == 31))
                hb = wpool.tile([32, 512], F16, tag="hb", name="hb")
                nc.scalar.activation(out=hb[:], in_=acc[:32, :], func=AF.Copy)
                for c in range(4):
                    hc = wpool.tile([128, 32], F16, tag="hc", name="hc")
                    transp(hb[:, 128 * c:128 * (c + 1)], hc[:])
                    s1 = wpool.tile([128, 1], F32, tag="b_s1", name="b_s1")
                    nc.vector.tensor_reduce(out=s1[:], in_=hc[:], axis=AX.X, op=ALU.add)
                    mu_ = wpool.tile([128, 1], F32, tag="b_mu", name="b_mu")
                    nc.vector.tensor_scalar_mul(mu_[:], s1[:], 1.0 / 32.0)
                    sq = wpool.tile([128, 32], F32, tag="b_sq", name="b_sq")
                    nc.vector.tensor_mul(sq[:], hc[:], hc[:])
                    s2_ = wpool.tile([128, 1], F32, tag="b_s2", name="b_s2")
                    nc.vector.tensor_reduce(out=s2_[:], in_=sq[:], axis=AX.X, op=ALU.add)
                    var = wpool.tile([128, 1], F32, tag="b_var", name="b_var")
                    nc.vector.scalar_tensor_tensor(out=var[:], in0=mu_[:], scalar=-1.0,
                                                   in1=mu_[:], op0=ALU.mult, op1=ALU.mult)
                    nc.vector.scalar_tensor_tensor(out=var[:], in0=s2_[:], scalar=1.0 / 32.0,
                                                   in1=var[:], op0=ALU.mult, op1=ALU.add)
                    sd = wpool.tile([128, 1], F32, tag="b_sd", name="b_sd")
                    nc.scalar.activation(out=sd[:], in_=var[:], func=AF.Sqrt, bias=eps_t[:, 0:1])
                    rs = wpool.tile([128, 1], F32, tag="b_rs", name="b_rs")
                    nc.vector.reciprocal(rs[:], sd[:])
                    a_ = wpool.tile([128, 1], F32, tag="b_a", name="b_a")
                    nc.vector.tensor_mul(a_[:], rs[:], g_t[:, c:c + 1])
                    sh = wpool.tile([128, 1], F32, tag="b_sh", name="b_sh")
                    nc.vector.scalar_tensor_tensor(out=sh[:], in0=mu_[:], scalar=-1.0,
                                                   in1=a_[:], op0=ALU.mult, op1=ALU.mult)
                    nc.vector.tensor_add(sh[:], sh[:], be_t[:, c:c + 1])
                    nc.scalar.activation(out=out_sb[:, 32 * c:32 * c + 32], in_=hc[:],
                                         func=AF.Relu, scale=a_[:, 0:1], bias=sh[:, 0:1])

            with nc.named_scope("mlp6"):
                h6 = bigpool.tile([128, 128], F16, tag="h6sb", name="h6sb")
                mlp_layer(6, act6, h6)
                nc.sync.dma_start(out=h6_loc.ap(), in_=h6[:])
                ag(h6_loc.ap(), h6_all.ap())
            with nc.named_scope("mlp7"):
                act7 = bigpool.tile([128, 1024], F16, tag="act7", name="act7")
                for r in range(8):
                    nc.sync.dma_start(out=act7[:, 128 * r:128 * (r + 1)],
                                      in_=h6_all[128 * r:128 * (r + 1), :])
                h7 = bigpool.tile([128, 128], F16, tag="h7sb", name="h7sb")
                mlp_layer(7, act7, h7)
                nc.sync.dma_start(out=h7_loc.ap(), in_=h7[:])
                ag(h7_loc.ap(), h7_all.ap())
            with nc.named_scope("mlp8"):
                act8 = bigpool.tile([128, 1024], F16, tag="act8", name="act8")
                for r in range(8):
                    nc.sync.dma_start(out=act8[:, 128 * r:128 * (r + 1)],
                                      in_=h7_all[128 * r:128 * (r + 1), :])
                h8 = bigpool.tile([128, 128], F16, tag="h8sb", name="h8sb")
                mlp_layer(8, act8, h8)

            with nc.named_scope("mlp9"):
                w9t = cpool.tile([128, 512], F16, tag="w9t", name="w9t")
                nc.sync.dma_start(out=w9t[:], in_=ein["w9"][:, :])
                acc9 = apool.tile([128, 512], F32, tag="acc", name="acc9")
                for c in range(4):
                    nc.tensor.matmul(out=acc9[:32, :128], lhsT=h8[:, 32 * c:32 * c + 32],
                                     rhs=w9t[:, 128 * c:128 * (c + 1)],
                                     start=(c == 0), stop=(c == 3))
                p9sb = wpool.tile([32, 128], F32, tag="p9sb", name="p9sb")
                nc.scalar.activation(out=p9sb[:], in_=acc9[:32, :128], func=AF.Copy)
                nc.sync.dma_start(out=p9_loc.ap(), in_=p9sb[:])
                ag(p9_loc.ap(), p9_all.ap())
                tot = wpool.tile([32, 128], F32, tag="f_tot", name="f_tot")
                nc.sync.dma_start(out=tot[:], in_=p9_all[0:32, :])
                for k in range(1, 8):
                    pk = wpool.tile([32, 128], F32, tag="f_pk", name="f_pk")
                    nc.sync.dma_start(out=pk[:], in_=p9_all[32 * k:32 * (k + 1), :])
                    nc.vector.tensor_add(tot[:], tot[:], pk[:])
                totT = wpool.tile([128, 32], F32, tag="f_totT", name="f_totT")
                pst = ppool.tile([128, 512], F32, tag="ps", name="pst")
                identf = cpool.tile([32, 32], F32, tag="identf", name="identf")
                nc.scalar.activation(out=identf[:], in_=ident[:32, :32], func=AF.Copy)
                nc.tensor.transpose(out=pst[:128, :32], in_=tot[:], identity=identf[:])
                nc.scalar.activation(out=totT[:], in_=pst[:128, :32], func=AF.Copy)
                s1 = wpool.tile([128, 1], F32, tag="f_s1", name="f_s1")
                nc.vector.tensor_reduce(out=s1[:], in_=totT[:], axis=AX.X, op=ALU.add)
                mu_ = wpool.tile([128, 1], F32, tag="f_mu", name="f_mu")
                nc.vector.tensor_scalar_mul(mu_[:], s1[:], 1.0 / 32.0)
                sq = wpool.tile([128, 32], F32, tag="f_sq", name="f_sq")
                nc.vector.tensor_mul(sq[:], totT[:], totT[:])
                s2_ = wpool.tile([128, 1], F32, tag="f_s2", name="f_s2")
                nc.vector.tensor_reduce(out=s2_[:], in_=sq[:], axis=AX.X, op=ALU.add)
                var = wpool.tile([128, 1], F32, tag="f_var", name="f_var")
                nc.vector.scalar_tensor_tensor(out=var[:], in0=mu_[:], scalar=-1.0,
                                               in1=mu_[:], op0=ALU.mult, op1=ALU.mult)
                nc.vector.scalar_tensor_tensor(out=var[:], in0=s2_[:], scalar=1.0 / 32.0,
                                               in1=var[:], op0=ALU.mult, op1=ALU.add)
                sdf = wpool.tile([128, 1], F32, tag="f_sd", name="f_sd")
                nc.scalar.activation(out=sdf[:], in_=var[:], func=AF.Sqrt, bias=eps_t[:, 0:1])
                rs = wpool.tile([128, 1], F32, tag="f_rs", name="f_rs")
                nc.vector.reciprocal(rs[:], sdf[:])
                neg = wpool.tile([128, 1], F32, tag="f_neg", name="f_neg")
                nc.vector.scalar_tensor_tensor(out=neg[:], in0=mu_[:], scalar=-1.0,
                                               in1=rs[:], op0=ALU.mult, op1=ALU.mult)
                outt = wpool.tile([128, 32], F32, tag="f_out", name="f_out")
                nc.scalar.activation(out=outt[:], in_=totT[:], func=AF.Identity,
                                     scale=rs[:, 0:1], bias=neg[:, 0:1])
                nc.sync.dma_start(out=out_mu[:, :], in_=outt[:])

    nc.compile()
    return nc


# ---------------------------------------------------------------- entry point
def kernel(**inputs) -> np.ndarray:
    per_core, meta = _host_prep(inputs)
    if "prog" not in _CACHE:
        _CACHE["prog"] = _build_nc(meta, per_core[0])
    nc = _CACHE["prog"]
    res = bass_utils.run_bass_kernel_spmd(nc, per_core, core_ids=list(range(NCORES)))
    return np.ascontiguousarray(res.results[0]["mu"].T)


# revision 32
# speedup vs baseline: 3.2710x; 1.0798x over previous
"""Trainium2 Bass kernel for nn_Encoder_base (5x ChebConv GNN + pool + MLP).

Distribution over 8 NeuronCores (all matmuls fp16, fp32 PSUM):
  - level 0: the two props the einsum needs (Tx1[l0] = S0[l0]@X0 and
    p2t = S0[l0]@S0@X0) are composed on the HOST into single operators on
    the input X0 (2-hop edge expansion M0 = S0[l0]*S0). Edge-major X0 rows
    are pregathered host-side -> the props are pure streaming selection
    matmuls: zero indirect DMA, zero full-graph AllGather.
  - level 1: stacked dense operator T = [P_l1; S1[l1]; M1=S1[l1]*S1]
    (3072 x 4096) applied to z1, dest-sharded (128 l1-positions/core, all
    1024 batch-features wide); einsum is dest-sharded too. Comm: one
    chunked z1 AllGather + one small z2 AllGather.
  - levels 2-3: batch-sharded (4 batches/core), dense-S matmuls,
    block-diagonal channel mixes in feature-major layout.
  - MLP: output-feature sharded (512 cols of W6/7/8, 512 rows of W9 per
    core); activations [128k,32] are the stationary lhsT, W streams as rhs;
    BatchNorm per-feature after a PE transpose; activations AllGathered.
"""
import numpy as np
import concourse.bass as bass
import concourse.bacc as bacc
import concourse.tile as tile
from concourse import mybir, bass_utils

F32 = mybir.dt.float32
F16 = mybir.dt.float16
I16 = mybir.dt.int16
AF = mybir.ActivationFunctionType
ALU = mybir.AluOpType
AX = mybir.AxisListType
RG = [list(range(8))]
NCORES = 8
N0, N1, N2, N3 = 16384, 4096, 1024, 128
EPS = 1e-5
H16 = np.float16

_CACHE = {}


# ---------------------------------------------------------------- host prep
def _prep_prop(row, col, we, n_dest, n_shard):
    """Sorted-by-dest edges -> 128-dest windows, 128-edge chunks, padded so
    chunk counts per window match across shards (one SPMD program).
    Emits per-chunk selection matrices sel[chunk, edge_local, dst_local]."""
    window = 128
    order = np.argsort(row, kind="stable")
    row, col, we = row[order], col[order], we[order]
    per = n_dest // n_shard
    nwin = per // window
    counts = np.zeros((n_shard, nwin), np.int64)
    lists = {}
    for s in range(n_shard):
        lo = s * per
        for wi in range(nwin):
            wlo = lo + wi * window
            a = np.searchsorted(row, wlo, side="left")
            b = np.searchsorted(row, wlo + window, side="left")
            lists[(s, wi)] = (row[a:b] - wlo, col[a:b], we[a:b])
            counts[s, wi] = (b - a + 127) // 128
    ncw = np.maximum(counts.max(axis=0), 1)
    C = int(ncw.sum())
    src = np.zeros((n_shard, C, 128), np.int64)
    dst = np.full((n_shard, C, 128), 200.0, np.float32)
    wea = np.zeros((n_shard, C, 128), np.float32)
    for s in range(n_shard):
        base = 0
        for wi in range(nwin):
            dl, cl, wl = lists[(s, wi)]
            n = len(dl)
            k = int(ncw[wi])
            src[s, base:base + k].reshape(-1)[:n] = cl
            ch = base + np.arange(n) // 128
            ep = np.arange(n) % 128
            dst[s, ch, ep] = dl
            wea[s, ch, ep] = wl
            base += k
    return [int(x) for x in ncw], src, dst, wea


def _edge_we(e, n):
    row, col = np.asarray(e[0], np.int64), np.asarray(e[1], np.int64)
    deg = np.bincount(row, minlength=n).astype(np.float32)
    dis = np.where(deg > 0, 1.0 / np.sqrt(np.maximum(deg, 1.0)), 0.0).astype(np.float32)
    return row, col, -(dis[row] * dis[col]).astype(np.float32)


def _sub_edges(row, col, we, pool_idx):
    order = np.argsort(row, kind="stable")
    row, col, we = row[order], col[order], we[order]
    starts = np.searchsorted(row, pool_idx, side="left")
    ends = np.searchsorted(row, pool_idx, side="right")
    nr, ncl, nw = [], [], []
    for i in range(len(pool_idx)):
        s, e = starts[i], ends[i]
        if e > s:
            nr.append(np.full(e - s, i, np.int64))
            ncl.append(col[s:e])
            nw.append(we[s:e])
    return np.concatenate(nr), np.concatenate(ncl), np.concatenate(nw)


def _twohop(ri, ci, wi, row, col, we, n):
    """(i,j,w1) sub-edges composed with full edges (j->k,w2): (i,k,w1*w2)."""
    order = np.argsort(row, kind="stable")
    rs, cs, ws = row[order], col[order], we[order]
    starts = np.searchsorted(rs, np.arange(n), side="left")
    ends = np.searchsorted(rs, np.arange(n), side="right")
    cnt = (ends - starts)[ci]
    I = np.repeat(ri, cnt)
    W1 = np.repeat(wi, cnt)
    base = np.repeat(starts[ci], cnt)
    within = np.arange(cnt.sum()) - np.repeat(np.cumsum(cnt) - cnt, cnt)
    offs = base + within
    return I, cs[offs], W1 * ws[offs]


def _dense_s(row, col, we, n, m):
    s = np.zeros((n, m), np.float32)
    np.add.at(s, (row, col), we)
    return s


def _tile_w(w, pack):
    """[K, M] -> [K//(128*pack) * 128, pack*M]: pack K-blocks side by side."""
    k, m = w.shape
    nb = k // 128
    t = w.reshape(nb // pack, pack, 128, m).transpose(0, 2, 1, 3)
    return np.ascontiguousarray(t.reshape((nb // pack) * 128, pack * m))


def _idx_tile(flat):
    """flat int idx list -> [128, len//16] int16 (16-part wrap, x8 replicas)."""
    return np.ascontiguousarray(
        np.tile(flat.astype(np.int16).reshape(-1, 16).T, (8, 1)))


def _chunk_tile(arr3):
    """[C, 128, W] -> [128, C*W] (chunk c at cols c*W..)."""
    C, _, W = arr3.shape
    return np.ascontiguousarray(
        arr3.transpose(1, 0, 2).reshape(128, C * W)).astype(H16)


def _host_prep(inputs):
    d = {k: np.asarray(v) for k, v in inputs.items()}
    x = d["x"].astype(np.float32)
    l0 = np.asarray(d["l0"], np.int64)
    l1 = np.asarray(d["l1"], np.int64)
    l2 = np.asarray(d["l2"], np.int64)

    X0 = np.ascontiguousarray(x.transpose(1, 0, 2).reshape(N0, 96))
    X0p = np.zeros((N0, 128), np.float32)
    X0p[:, :96] = X0
    X0l0T = np.ascontiguousarray(X0[l0].T)  # [96, 4096]

    # level-0 operators on X0: a = S0[l0] (tap1), m = S0[l0]@S0 (tap2)
    r0, c0, w0 = _edge_we(d["e0"], N0)
    r0s, c0s, w0s = _sub_edges(r0, c0, w0, l0)
    ncw_a, src_a, dst_a, we_a = _prep_prop(r0s, c0s, w0s, N1, NCORES)
    mI, mK, mW = _twohop(r0s, c0s, w0s, r0, c0, w0, N0)
    ncw_m, src_m, dst_m, we_m = _prep_prop(mI, mK, mW, N1, NCORES)

    # level-1 stacked operator T = [P_l1; S1[l1]; M1]
    r1, c1, w1 = _edge_we(d["e1"], N1)
    S1 = _dense_s(r1, c1, w1, N1, N1)
    r1s, c1s, w1s = _sub_edges(r1, c1, w1, l1)
    S1l1 = _dense_s(r1s, c1s, w1s, N2, N1)    # [1024, 4096]
    M1 = S1l1 @ S1                            # [1024, 4096]
    P_l1 = np.zeros((N2, N1), np.float32)
    P_l1[np.arange(N2), l1] = 1.0
    Tblocks = [P_l1, S1l1, M1]

    r2, c2, w2 = _edge_we(d["e2"], N2)
    S2 = _dense_s(r2, c2, w2, N2, N2)
    S2T = _tile_w(np.ascontiguousarray(S2.T), 8).astype(H16)       # [128, 8192]
    S2l2T = _tile_w(np.ascontiguousarray(S2[l2].T), 8).astype(H16)  # [128, 1024]
    P_l2 = np.zeros((N2, 128), np.float32)
    P_l2[l2, np.arange(128)] = 1.0
    P_l2 = _tile_w(P_l2, 8).astype(H16)                             # [128, 1024]

    r3, c3, w3 = _edge_we(d["e3"], N3)
    S3T = np.ascontiguousarray(_dense_s(r3, c3, w3, N3, N3).T).astype(H16)

    def wmod(W):
        return W[0] - W[2], W[1], 2.0 * W[2]

    Wm1 = wmod(d["Wc1"].astype(np.float32))
    Wm = [wmod(d[f"Wc{i}"].astype(np.float32)) for i in (2, 3, 4, 5)]
    eye4 = np.eye(4, dtype=np.float32)

    per_core = []
    for k in range(NCORES):
        m = {}
        m["identbf"] = np.eye(128, dtype=np.float32).astype(H16)
        m["iota"] = np.tile(np.arange(128, dtype=np.float32), (128, 1))
        m["epsv"] = np.full((128, 1), EPS, np.float32)
        m["warm"] = np.zeros((1, 8), np.float32)
        m["X0l0Tw"] = np.ascontiguousarray(
            X0l0T[:, 512 * k:512 * (k + 1)]).astype(H16)
        m["Xg_a"] = _chunk_tile(X0p[src_a[k]].astype(H16))
        m["a_dst"] = np.ascontiguousarray(dst_a[k].T)
        m["a_we"] = np.ascontiguousarray(we_a[k].T)
        m["Xg_m"] = _chunk_tile(X0p[src_m[k]].astype(H16))
        m["m_dst"] = np.ascontiguousarray(dst_m[k].T)
        m["m_we"] = np.ascontiguousarray(we_m[k].T)
        # stacked-T lhsT chunks: block b, k-chunk kk at cols (b*32+kk)*128
        tt = np.zeros((128, 96 * 128), np.float32)
        for b, blk in enumerate(Tblocks):
            bt = blk[128 * k:128 * (k + 1), :].T  # [4096, 128]
            for kk in range(32):
                tt[:, (b * 32 + kk) * 128:(b * 32 + kk + 1) * 128] = \
                    bt[128 * kk:128 * (kk + 1), :]
        m["Tt"] = tt.astype(H16)
        # z2n extraction: for node-chunk c, rows 1024c + 128k + j of z2T_all
        m["z2n_idx"] = _idx_tile(np.concatenate(
            [1024 * c + 128 * k + np.arange(128) for c in range(8)]))
        m["S2T"] = S2T
        m["S2l2T"] = S2l2T
        m["P_l2"] = P_l2
        m["S3T"] = S3T
        for g in range(8):
            for t in range(3):
                bw = np.zeros((96, 128), np.float32)
                for j in range(4):
                    bg = 4 * g + j
                    bw[3 * bg:3 * bg + 3, 32 * j:32 * j + 32] = Wm1[t]
                m[f"bigw0_{g}_{t}"] = bw.astype(H16)
        for lev in range(4):
            for t in range(3):
                m[f"bigw{lev + 1}_{t}"] = np.kron(eye4, Wm[lev][t]).astype(H16)
        for lev, nm in ((1, "b1"), (2, "b2"), (3, "b3"), (4, "b4"), (5, "b5")):
            m[f"bias{lev}"] = np.tile(d[nm].astype(np.float32), 4).reshape(128, 1)
        for li in (6, 7, 8):
            W = d[f"W{li}"].astype(np.float32)[:, 512 * k:512 * k + 512]
            m[f"w{li}"] = np.ascontiguousarray(
                W.reshape(32, 128, 512).transpose(1, 0, 2).reshape(128, 32 * 512)
            ).astype(H16)
            m[f"g{li}"] = np.ascontiguousarray(
                d[f"g{li}"].astype(np.float32)[512 * k:512 * k + 512].reshape(4, 128).T)
            m[f"be{li}"] = np.ascontiguousarray(
                d[f"be{li}"].astype(np.float32)[512 * k:512 * k + 512].reshape(4, 128).T)
        W9 = d["W9"].astype(np.float32)[512 * k:512 * k + 512]  # [512, 128]
        m["w9"] = np.ascontiguousarray(
            W9.reshape(4, 128, 128).transpose(1, 0, 2).reshape(128, 512)).astype(H16)
        per_core.append(m)

    meta = {"a": ncw_a, "m": ncw_m}
    return per_core, meta


# ---------------------------------------------------------------- device program
def _build_nc(meta, shapes):
    nc = bacc.Bacc("TRN2", target_bir_lowering=False, debug=False, num_devices=NCORES)
    ein = {}
    for name, arr in shapes.items():
        dt = {np.dtype(np.int16): I16, np.dtype(H16): F16,
              np.dtype(np.float32): F32}[arr.dtype]
        ein[name] = nc.dram_tensor(name, list(arr.shape), dt, kind="ExternalInput")
    out_mu = nc.dram_tensor("mu", [128, 32], F32, kind="ExternalOutput")

    warm_all = nc.dram_tensor("warm_all", [8, 8], F32)
    warm_loc = nc.dram_tensor("warm_loc", [1, 8], F32)
    z1c = [nc.dram_tensor(f"z1c_{i}", [128, 1024], F16) for i in range(4)]
    z1ag = [nc.dram_tensor(f"z1ag_{i}", [1024, 1024], F16) for i in range(4)]
    z2T_loc = nc.dram_tensor("z2T_loc", [1024, 128], F16)
    z2T_all = nc.dram_tensor("z2T_all", [8192, 128], F16)
    x_loc = nc.dram_tensor("x_loc", [4, 4096], F16)
    x_all = nc.dram_tensor("x_all", [32, 4096], F16)
    h6_loc = nc.dram_tensor("h6_loc", [128, 128], F16)
    h6_all = nc.dram_tensor("h6_all", [1024, 128], F16)
    h7_loc = nc.dram_tensor("h7_loc", [128, 128], F16)
    h7_all = nc.dram_tensor("h7_all", [1024, 128], F16)
    p9_loc = nc.dram_tensor("p9_loc", [32, 128], F32)
    p9_all = nc.dram_tensor("p9_all", [256, 128], F32)

    def ag(loc_ap, all_ap):
        nc.gpsimd.collective_compute(
            "AllGather", ALU.bypass, replica_groups=RG,
            ins=[loc_ap.opt()], outs=[all_ap.opt()])

    with tile.TileContext(nc) as tc:
        with (
            tc.tile_pool(name="const", bufs=1) as cpool,
            tc.tile_pool(name="big", bufs=1) as bigpool,
            tc.tile_pool(name="work", bufs=3) as wpool,
            tc.tile_pool(name="wload", bufs=2) as wlpool,
            tc.tile_pool(name="psA", bufs=3, space="PSUM") as ppool,
            tc.tile_pool(name="psT", bufs=2, space="PSUM") as tpool,
            tc.tile_pool(name="psB", bufs=1, space="PSUM") as apool,
        ):
            ident = cpool.tile([128, 128], F16, tag="identbf", name="identbf")
            nc.sync.dma_start(out=ident[:], in_=ein["identbf"][:, :])
            iota_t = cpool.tile([128, 128], F32, tag="iota", name="iota")
            nc.sync.dma_start(out=iota_t[:], in_=ein["iota"][:, :])
            eps_t = cpool.tile([128, 1], F32, tag="epsv", name="epsv")
            nc.sync.dma_start(out=eps_t[:], in_=ein["epsv"][:, :])

            def load_const(name, dt=F16):
                t = cpool.tile(list(shapes[name].shape), dt, tag=name)
                nc.sync.dma_start(out=t[:], in_=ein[name][:, :])
                return t

            GRP = 16

            def grp_load(pref, g0, gc, tag, eng=None, grp=None):
                sl = wpool.tile([128, (grp or GRP) * 128], F16, tag=tag,
                                name=tag, bufs=2)
                (eng or nc.sync).dma_start(out=sl[:, :gc * 128],
                                           in_=ein[pref][:, g0 * 128:(g0 + gc) * 128])
                return sl

            def transp(src_ap, dst_ap):
                p, f = src_ap.shape
                ps = tpool.tile([128, 128], F16, tag="tp", name="tp")
                nc.tensor.transpose(out=ps[:f, :p], in_=src_ap, identity=ident[:p, :p])
                nc.scalar.activation(out=dst_ap, in_=ps[:f, :p], func=AF.Copy)

            def einsum_win(bigw, taps, Din, width, out_ap, func, bias_ap):
                ps = ppool.tile([128, 512], F32, tag="ps", name="ps")
                for t in range(3):
                    nc.tensor.matmul(out=ps[:, :width], lhsT=bigw[t][:Din, :],
                                     rhs=taps[t], start=(t == 0), stop=(t == 2))
                f2 = AF.Identity if func == AF.Copy else func
                nc.scalar.activation(out=out_ap, in_=ps[:, :width], func=f2, bias=bias_ap)

            # warm up the CC ring while level-0 computes
            with nc.named_scope("warmup"):
                warm = wpool.tile([1, 8], F32, tag="warm", name="warm")
                nc.sync.dma_start(out=warm[:], in_=ein["warm"][:, :])
                nc.sync.dma_start(out=warm_loc.ap(), in_=warm[:])
                ag(warm_loc.ap(), warm_all.ap())

            # ====== LEVEL 0: per-window pipeline of props -> einsum -> AG ====
            # tap1 = S0[l0]@X0, tap2 = (S0[l0]@S0)@X0, then the channel-mix
            # einsum for window wi immediately, then AllGather that window.
            with nc.named_scope("l0"):
                Ca, Cm = sum(meta["a"]), sum(meta["m"])
                cura = {"g0": -1}
                curm = {"g0": -1}

                def get_grp(cur, cc, C, xg, xtag, grp):
                    g0 = (cc // grp) * grp
                    if g0 != cur["g0"]:
                        gc = min(grp, C - g0)
                        cur["g0"] = g0
                        cur["x"] = grp_load(xg, g0, gc, xtag, eng=nc.scalar, grp=grp)
                    return cur, (cc - cur["g0"]) * 128

                def mk_sel(dw_t, cc):
                    sel = wpool.tile([128, 128], F16, tag="sel", name="sel", bufs=4)
                    nc.vector.tensor_scalar(
                        out=sel[:], in0=iota_t[:], scalar1=dw_t[0][:, cc:cc + 1],
                        scalar2=dw_t[1][:, cc:cc + 1], op0=ALU.is_equal, op1=ALU.mult)
                    return sel

                adw = [load_const("a_dst", F32), load_const("a_we", F32)]
                mdw = [load_const("m_dst", F32), load_const("m_we", F32)]

                bias1 = load_const("bias1", F32)
                x0w = cpool.tile([96, 512], F16, tag="X0l0Tw", name="X0l0Tw")
                nc.sync.dma_start(out=x0w[:], in_=ein["X0l0Tw"][:, :])
                bw0g = [[load_const(f"bigw0_{g}_{t}") for t in range(3)]
                        for g in range(8)]
                tap1_sb = bigpool.tile([96, 512], F16, tag="tap1_sb", name="tap1_sb")
                tap2_sb = bigpool.tile([96, 512], F16, tag="tap2_sb", name="tap2_sb")
                ba, bm = 0, 0
                for wi in range(4):
                    psa = ppool.tile([128, 512], F32, tag="ps", name="ps")
                    for c in range(meta["a"][wi]):
                        g, lo = get_grp(cura, ba + c, Ca, "Xg_a", "xga", 8)
                        nc.tensor.matmul(out=psa[:96, :128], lhsT=g["x"][:, lo:lo + 96],
                                         rhs=mk_sel(adw, ba + c)[:],
                                         start=(c == 0), stop=(c == meta["a"][wi] - 1))
                    nc.scalar.activation(out=tap1_sb[:, wi * 128:(wi + 1) * 128],
                                         in_=psa[:96, :128], func=AF.Copy)
                    ba += meta["a"][wi]
                    psm = ppool.tile([128, 512], F32, tag="ps", name="ps")
                    for c in range(meta["m"][wi]):
                        g, lo = get_grp(curm, bm + c, Cm, "Xg_m", "xgm", 16)
                        nc.tensor.matmul(out=psm[:96, :128], lhsT=g["x"][:, lo:lo + 96],
                                         rhs=mk_sel(mdw, bm + c)[:],
                                         start=(c == 0), stop=(c == meta["m"][wi] - 1))
                    nc.scalar.activation(out=tap2_sb[:, wi * 128:(wi + 1) * 128],
                                         in_=psm[:96, :128], func=AF.Copy)
                    bm += meta["m"][wi]
                    for g in range(8):
                        z1gT = wpool.tile([128, 128], F16, tag="z1Tw", name="z1Tw")
                        einsum_win(bw0g[g],
                                   [x0w[:, wi * 128:(wi + 1) * 128],
                                    tap1_sb[:96, wi * 128:(wi + 1) * 128],
                                    tap2_sb[:96, wi * 128:(wi + 1) * 128]],
                                   96, 128, z1gT[:], AF.Copy, bias1[:, 0:1])
                        t = wpool.tile([128, 128], F16, tag="z1nc", name="z1nc")
                        transp(z1gT[:], t[:])
                        nc.sync.dma_start(
                            out=z1c[wi][:, 128 * g:128 * (g + 1)], in_=t[:])
                    with nc.named_scope(f"agz1_{wi}"):
                        ag(z1c[wi].ap(), z1ag[wi].ap())

            # ====== LEVEL 1: taps = T @ z1, dest-sharded (128 pos x 1024) ====
            with nc.named_scope("l1_T"):
                tt = cpool.tile([128, 96 * 128], F16, tag="Tt", name="Tt")
                nc.sync.dma_start(out=tt[:, :6144], in_=ein["Tt"][:, :6144])
                nc.sync.dma_start(out=tt[:, 6144:], in_=ein["Tt"][:, 6144:])
                accs = [bigpool.tile([128, 1024], F16, tag=f"accT{b}", name=f"accT{b}")
                        for b in range(3)]
                for part in range(4):
                    for h in range(2):
                        z1sc = wpool.tile([128, 4096], F16, tag="z1sc", name="z1sc", bufs=2)
                        nc.sync.dma_start(
                            out=z1sc[:].rearrange("p (q d) -> p q d", d=1024),
                            in_=z1ag[part].ap()[512 * h:512 * (h + 1)]
                                .rearrange("(q p) d -> p q d", p=128))
                        for b in range(3):
                            psa = ppool.tile([128, 512], F32, tag="ps", name="ps")
                            psb = ppool.tile([128, 512], F32, tag="ps", name="ps")
                            for q in range(4):
                                kk = 4 * (4 * h + q) + part
                                lh = tt[:, (b * 32 + kk) * 128:(b * 32 + kk + 1) * 128]
                                nc.tensor.matmul(out=psa[:, :512], lhsT=lh,
                                                 rhs=z1sc[:, q * 1024:q * 1024 + 512],
                                                 start=(q == 0), stop=(q == 3))
                                nc.tensor.matmul(out=psb[:, :512], lhsT=lh,
                                                 rhs=z1sc[:, q * 1024 + 512:(q + 1) * 1024],
                                                 start=(q == 0), stop=(q == 3))
                            if part == 0 and h == 0:
                                nc.scalar.activation(out=accs[b][:, :512], in_=psa[:, :512],
                                                     func=AF.Copy)
                                nc.scalar.activation(out=accs[b][:, 512:], in_=psb[:, :512],
                                                     func=AF.Copy)
                            else:
                                nc.vector.tensor_add(accs[b][:, :512], accs[b][:, :512],
                                                     psa[:, :512])
                                nc.vector.tensor_add(accs[b][:, 512:], accs[b][:, 512:],
                                                     psb[:, :512])

            # ============ LEVEL 1 einsum (dest-sharded) -> z2T ===============
            with nc.named_scope("l1_einsum"):
                bw1 = [load_const(f"bigw1_{t}") for t in range(3)]
                bias2 = load_const("bias2", F32)
                tapTs = []
                for b in range(3):
                    tapT = bigpool.tile([128, 1024], F16, tag=f"tapT{b}", name=f"tapT{b}")
                    for f in range(8):
                        transp(accs[b][:, 128 * f:128 * (f + 1)],
                               tapT[:, 128 * f:128 * (f + 1)])
                    tapTs.append(tapT)
                for fg in range(8):
                    z2fg = wpool.tile([128, 128], F16, tag="z2fg", name="z2fg")
                    einsum_win(bw1, [tapTs[0][:, 128 * fg:128 * (fg + 1)],
                                     tapTs[1][:, 128 * fg:128 * (fg + 1)],
                                     tapTs[2][:, 128 * fg:128 * (fg + 1)]],
                               128, 128, z2fg[:], AF.Tanh, bias2[:, 0:1])
                    nc.sync.dma_start(out=z2T_loc[128 * fg:128 * (fg + 1), :],
                                      in_=z2fg[:])
            with nc.named_scope("ag_z2"):
                ag(z2T_loc.ap(), z2T_all.ap())

            # ====== z2n assembly (batch-sharded node-major) ==================
            z2n = bigpool.tile([128, 8 * 128], F16, tag="z2n", name="z2n")
            with nc.named_scope("z2n_asm"):
                z2i = cpool.tile([128, 8 * 8], I16, tag="z2i", name="z2i")
                nc.sync.dma_start(out=z2i[:], in_=ein["z2n_idx"][:, :])
                zb = wpool.tile([128, 1024], F16, tag="zb", name="zb", bufs=1)
                nc.gpsimd.dma_gather(
                    out_ap=zb[:].rearrange("p (c e) -> p c e", e=128),
                    in_ap=z2T_all[:, :],
                    idxs_ap=z2i[:],
                    num_idxs=1024, num_idxs_reg=1024, elem_size=128,
                    single_packet=False)
                for ci in range(8):
                    transp(zb[:, ci * 128:(ci + 1) * 128],
                           z2n[:, ci * 128:(ci + 1) * 128])

            # ================= LEVEL 2 (dense) =================
            with nc.named_scope("l2"):
                t1_l2 = bigpool.tile([128, 8 * 128], F16, tag="t1_l2", name="t1_l2")
                for half in range(2):
                    s2t = wlpool.tile([128, 4096], F16, tag="wld", name="wld")
                    nc.sync.dma_start(out=s2t[:], in_=ein["S2T"][:, 4096 * half:4096 * (half + 1)])
                    for dc in range(8):
                        ps = ppool.tile([128, 512], F32, tag="ps", name="ps")
                        for kk in range(4):
                            kc = half * 4 + kk
                            nc.tensor.matmul(
                                out=ps[:, :128],
                                lhsT=s2t[:, kk * 1024 + dc * 128: kk * 1024 + dc * 128 + 128],
                                rhs=z2n[:, kc * 128:(kc + 1) * 128],
                                start=(kk == 0), stop=(kk == 3))
                        if half == 0:
                            nc.scalar.activation(out=t1_l2[:, dc * 128:(dc + 1) * 128],
                                                 in_=ps[:, :128], func=AF.Copy)
                        else:
                            nc.vector.tensor_add(t1_l2[:, dc * 128:(dc + 1) * 128],
                                                 t1_l2[:, dc * 128:(dc + 1) * 128],
                                                 ps[:, :128])
                s2l2 = cpool.tile([128, 1024], F16, tag="s2l2", name="s2l2")
                nc.sync.dma_start(out=s2l2[:], in_=ein["S2l2T"][:, :])
                ps = ppool.tile([128, 512], F32, tag="ps", name="ps")
                for kc in range(8):
                    nc.tensor.matmul(out=ps[:, :128], lhsT=s2l2[:, kc * 128:(kc + 1) * 128],
                                     rhs=t1_l2[:, kc * 128:(kc + 1) * 128],
                                     start=(kc == 0), stop=(kc == 7))
                p2n_l2 = wpool.tile([128, 128], F16, tag="p2n_l2", name="p2n_l2")
                nc.scalar.activation(out=p2n_l2[:], in_=ps[:, :128], func=AF.Copy)
                pl2 = cpool.tile([128, 1024], F16, tag="pl2", name="pl2")
                nc.sync.dma_start(out=pl2[:], in_=ein["P_l2"][:, :])
                z2l2T = wpool.tile([128, 128], F16, tag="z2l2T", name="z2l2T")
                psg = ppool.tile([128, 512], F32, tag="ps", name="ps")
                for kc in range(8):
                    nc.tensor.matmul(out=psg[:, :128], lhsT=z2n[:, kc * 128:(kc + 1) * 128],
                                     rhs=pl2[:, kc * 128:(kc + 1) * 128],
                                     start=(kc == 0), stop=(kc == 7))
                nc.scalar.activation(out=z2l2T[:], in_=psg[:, :128], func=AF.Copy)
                t1l2T = wpool.tile([128, 128], F16, tag="t1l2T", name="t1l2T")
                psg2 = ppool.tile([128, 512], F32, tag="ps", name="ps")
                for kc in range(8):
                    nc.tensor.matmul(out=psg2[:, :128], lhsT=t1_l2[:, kc * 128:(kc + 1) * 128],
                                     rhs=pl2[:, kc * 128:(kc + 1) * 128],
                                     start=(kc == 0), stop=(kc == 7))
                nc.scalar.activation(out=t1l2T[:], in_=psg2[:, :128], func=AF.Copy)
                p2l2T = wpool.tile([128, 128], F16, tag="p2l2T", name="p2l2T")
                transp(p2n_l2[:], p2l2T[:])
                bw2 = [load_const(f"bigw2_{t}") for t in range(3)]
                bias3 = load_const("bias3", F32)
                z3T = wpool.tile([128, 128], F16, tag="z3T", name="z3T")
                einsum_win(bw2, [z2l2T[:], t1l2T[:], p2l2T[:]], 128, 128,
                           z3T[:], AF.Tanh, bias3[:, 0:1])
                z3n = wpool.tile([128, 128], F16, tag="z3n", name="z3n")
                transp(z3T[:], z3n[:])

            # ================= LEVEL 3 =================
            with nc.named_scope("l3"):
                s3t = cpool.tile([128, 128], F16, tag="s3t", name="s3t")
                nc.sync.dma_start(out=s3t[:], in_=ein["S3T"][:, :])
                bias4 = load_const("bias4", F32)
                bias5 = load_const("bias5", F32)

                def conv_l3(zn, zT, bw_pref, bias_t, func, keep):
                    t1T = wpool.tile([128, 128], F16, tag=keep + "t1T", name=keep + "t1T")
                    ps = ppool.tile([128, 512], F32, tag="ps", name="ps")
                    nc.tensor.matmul(out=ps[:, :128], lhsT=zn, rhs=s3t[:], start=True, stop=True)
                    nc.scalar.activation(out=t1T[:], in_=ps[:, :128], func=AF.Copy)
                    t1n_ = wpool.tile([128, 128], F16, tag=keep + "t1n", name=keep + "t1n")
                    transp(t1T[:], t1n_[:])
                    p2T_ = wpool.tile([128, 128], F16, tag=keep + "p2T", name=keep + "p2T")
                    ps2 = ppool.tile([128, 512], F32, tag="ps", name="ps")
                    nc.tensor.matmul(out=ps2[:, :128], lhsT=t1n_[:], rhs=s3t[:], start=True, stop=True)
                    nc.scalar.activation(out=p2T_[:], in_=ps2[:, :128], func=AF.Copy)
                    bw = [load_const(f"{bw_pref}_{t}") for t in range(3)]
                    outT = wpool.tile([128, 128], F16, tag=keep + "oT", name=keep + "oT")
                    einsum_win(bw, [zT, t1T[:], p2T_[:]], 128, 128, outT[:], func, bias_t[:, 0:1])
                    outn = wpool.tile([128, 128], F16, tag=keep + "on", name=keep + "on")
                    transp(outT[:], outn[:])
                    return outn, outT

                z4n, z4T = conv_l3(z3n[:], z3T[:], "bigw3", bias4, AF.Tanh, "c4")
                o5n, o5T = conv_l3(z4n[:], z4T[:], "bigw4", bias5, AF.Copy, "c5")

            # ================= MLP input assembly =================
            with nc.named_scope("mlp_in"):
                nc.sync.dma_start(
                    out=x_loc.ap().rearrange("b (n c) -> n b c", c=32),
                    in_=o5n[:].rearrange("n (b c) -> n b c", c=32))
                ag(x_loc.ap(), x_all.ap())
                xT_sb = bigpool.tile([32, 4096], F16, tag="xT_sb", name="xT_sb")
                nc.sync.dma_start(out=xT_sb[:], in_=x_all[:, :])
                act6 = bigpool.tile([128, 1024], F16, tag="act6", name="act6")
                for i in range(32):
                    transp(xT_sb[:, 128 * i:128 * (i + 1)], act6[:, 32 * i:32 * i + 32])

            # ================= MLP =================
            def mlp_layer(li, act_sb, out_sb):
                g_t = load_const(f"g{li}", F32)
                be_t = load_const(f"be{li}", F32)
                wt = wlpool.tile([128, 32 * 512], F16, tag="wld", name="wld")
                nc.sync.dma_start(out=wt[:, :8192], in_=ein[f"w{li}"][:, :8192])
                nc.sync.dma_start(out=wt[:, 8192:], in_=ein[f"w{li}"][:, 8192:])
                acc = apool.tile([128, 512], F32, tag="acc", name="acc")
                for k2 in range(32):
                    nc.tensor.matmul(out=acc[:32, :], lhsT=act_sb[:, 32 * k2:32 * k2 + 32],
                                     rhs=wt[:, 512 * k2:512 * (k2 + 1)],
                                     start=(k2 == 0), stop=(k2 == 31))
                hb = wpool.tile([32, 512], F16, tag="hb", name="hb")
                nc.scalar.activation(out=hb[:], in_=acc[:32, :], func=AF.Copy)
                for c in range(4):
                    hc = wpool.tile([128, 32], F16, tag="hc", name="hc")
                    transp(hb[:, 128 * c:128 * (c + 1)], hc[:])
                    s1 = wpool.tile([128, 1], F32, tag="b_s1", name="b_s1")
                    nc.vector.tensor_reduce(out=s1[:], in_=hc[:], axis=AX.X, op=ALU.add)
                    mu_ = wpool.tile([128, 1], F32, tag="b_mu", name="b_mu")
                    nc.vector.tensor_scalar_mul(mu_[:], s1[:], 1.0 / 32.0)
                    sq = wpool.tile([128, 32], F32, tag="b_sq", name="b_sq")
                    nc.vector.tensor_mul(sq[:], hc[:], hc[:])
                    s2_ = wpool.tile([128, 1], F32, tag="b_s2", name="b_s2")
                    nc.vector.tensor_reduce(out=s2_[:], in_=sq[:], axis=AX.X, op=ALU.add)
                    var = wpool.tile([128, 1], F32, tag="b_var", name="b_var")
                    nc.vector.scalar_tensor_tensor(out=var[:], in0=mu_[:], scalar=-1.0,
                                                   in1=mu_[:], op0=ALU.mult, op1=ALU.mult)
                    nc.vector.scalar_tensor_tensor(out=var[:], in0=s2_[:], scalar=1.0 / 32.0,
                                                   in1=var[:], op0=ALU.mult, op1=ALU.add)
                    sd = wpool.tile([128, 1], F32, tag="b_sd", name="b_sd")
                    nc.scalar.activation(out=sd[:], in_=var[:], func=AF.Sqrt, bias=eps_t[:, 0:1])
                    rs = wpool.tile([128, 1], F32, tag="b_rs", name="b_rs")
                    nc.vector.reciprocal(rs[:], sd[:])
                    a_ = wpool.tile([128, 1], F32, tag="b_a", name="b_a")
                    nc.vector.tensor_mul(a_[:], rs[:], g_t[:, c:c + 1])
                    sh = wpool.tile([128, 1], F32, tag="b_sh", name="b_sh")
                    nc.vector.scalar_tensor_tensor(out=sh[:], in0=mu_[:], scalar=-1.0,
                                                   in1=a_[:], op0=ALU.mult, op1=ALU.mult)
                    nc.vector.tensor_add(sh[:], sh[:], be_t[:, c:c + 1])
                    nc.scalar.activation(out=out_sb[:, 32 * c:32 * c + 32], in_=hc[:],
                                         func=AF.Relu, scale=a_[:, 0:1], bias=sh[:, 0:1])

            with nc.named_scope("mlp6"):
                h6 = bigpool.tile([128, 128], F16, tag="h6sb", name="h6sb")
                mlp_layer(6, act6, h6)
                nc.sync.dma_start(out=h6_loc.ap(), in_=h6[:])
                ag(h6_loc.ap(), h6_all.ap())
            with nc.named_scope("mlp7"):
                act7 = bigpool.tile([128, 1024], F16, tag="act7", name="act7")
                for r in range(8):
                    nc.sync.dma_start(out=act7[:, 128 * r:128 * (r + 1)],
                                      in_=h6_all[128 * r:128 * (r + 1), :])
                h7 = bigpool.tile([128, 128], F16, tag="h7sb", name="h7sb")
                mlp_layer(7, act7, h7)
                nc.sync.dma_start(out=h7_loc.ap(), in_=h7[:])
                ag(h7_loc.ap(), h7_all.ap())
            with nc.named_scope("mlp8"):
                act8 = bigpool.tile([128, 1024], F16, tag="act8", name="act8")
                for r in range(8):
                    nc.sync.dma_start(out=act8[:, 128 * r:128 * (r + 1)],
                                      in_=h7_all[128 * r:128 * (r + 1), :])
                h8 = bigpool.tile([128, 128], F16, tag="h8sb", name="h8sb")
                mlp_layer(8, act8, h8)

            with nc.named_scope("mlp9"):
                w9t = cpool.tile([128, 512], F16, tag="w9t", name="w9t")
                nc.sync.dma_start(out=w9t[:], in_=ein["w9"][:, :])
                acc9 = apool.tile([128, 512], F32, tag="acc", name="acc9")
                for c in range(4):
                    nc.tensor.matmul(out=acc9[:32, :128], lhsT=h8[:, 32 * c:32 * c + 32],
                                     rhs=w9t[:, 128 * c:128 * (c + 1)],
                                     start=(c == 0), stop=(c == 3))
                p9sb = wpool.tile([32, 128], F32, tag="p9sb", name="p9sb")
                nc.scalar.activation(out=p9sb[:], in_=acc9[:32, :128], func=AF.Copy)
                nc.sync.dma_start(out=p9_loc.ap(), in_=p9sb[:])
                ag(p9_loc.ap(), p9_all.ap())
                tot = wpool.tile([32, 128], F32, tag="f_tot", name="f_tot")
                nc.sync.dma_start(out=tot[:], in_=p9_all[0:32, :])
                for k in range(1, 8):
                    pk = wpool.tile([32, 128], F32, tag="f_pk", name="f_pk")
                    nc.sync.dma_start(out=pk[:], in_=p9_all[32 * k:32 * (k + 1), :])
                    nc.vector.tensor_add(tot[:], tot[:], pk[:])
                totT = wpool.tile([128, 32], F32, tag="f_totT", name="f_totT")
                pst = ppool.tile([128, 512], F32, tag="ps", name="pst")
                identf = cpool.tile([32, 32], F32, tag="identf", name="identf")
                nc.scalar.activation(out=identf[:], in_=ident[:32, :32], func=AF.Copy)
                nc.tensor.transpose(out=pst[:128, :32], in_=tot[:], identity=identf[:])
                nc.scalar.activation(out=totT[:], in_=pst[:128, :32], func=AF.Copy)
                s1 = wpool.tile([128, 1], F32, tag="f_s1", name="f_s1")
                nc.vector.tensor_reduce(out=s1[:], in_=totT[:], axis=AX.X, op=ALU.add)
                mu_ = wpool.tile([128, 1], F32, tag="f_mu", name="f_mu")
                nc.vector.tensor_scalar_mul(mu_[:], s1[:], 1.0 / 32.0)
                sq = wpool.tile([128, 32], F32, tag="f_sq", name="f_sq")
                nc.vector.tensor_mul(sq[:], totT[:], totT[:])
                s2_ = wpool.tile([128, 1], F32, tag="f_s2", name="f_s2")
                nc.vector.tensor_reduce(out=s2_[:], in_=sq[:], axis=AX.X, op=ALU.add)
                var = wpool.tile([128, 1], F32, tag="f_var", name="f_var")
                nc.vector.scalar_tensor_tensor(out=var[:], in0=mu_[:], scalar=-1.0,
                                               in1=mu_[:], op0=ALU.mult, op1=ALU.mult)
                nc.vector.scalar_tensor_tensor(out=var[:], in0=s2_[:], scalar=1.0 / 32.0,
                                               in1=var[:], op0=ALU.mult, op1=ALU.add)
                sdf = wpool.tile([128, 1], F32, tag="f_sd", name="f_sd")
                nc.scalar.activation(out=sdf[:], in_=var[:], func=AF.Sqrt, bias=eps_t[:, 0:1])
                rs = wpool.tile([128, 1], F32, tag="f_rs", name="f_rs")
                nc.vector.reciprocal(rs[:], sdf[:])
                neg = wpool.tile([128, 1], F32, tag="f_neg", name="f_neg")
                nc.vector.scalar_tensor_tensor(out=neg[:], in0=mu_[:], scalar=-1.0,
                                               in1=rs[:], op0=ALU.mult, op1=ALU.mult)
                outt = wpool.tile([128, 32], F32, tag="f_out", name="f_out")
                nc.scalar.activation(out=outt[:], in_=totT[:], func=AF.Identity,
                                     scale=rs[:, 0:1], bias=neg[:, 0:1])
                nc.sync.dma_start(out=out_mu[:, :], in_=outt[:])

    nc.compile()
    return nc


# ---------------------------------------------------------------- entry point
def kernel(**inputs) -> np.ndarray:
    per_core, meta = _host_prep(inputs)
    if "prog" not in _CACHE:
        _CACHE["prog"] = _build_nc(meta, per_core[0])
    nc = _CACHE["prog"]
    res = bass_utils.run_bass_kernel_spmd(nc, per_core, core_ids=list(range(NCORES)))
    return np.ascontiguousarray(res.results[0]["mu"].T)


# revision 34
# speedup vs baseline: 3.4096x; 1.0424x over previous
"""Trainium2 Bass kernel for nn_Encoder_base (5x ChebConv GNN + pool + MLP).

Distribution over 8 NeuronCores (all matmuls fp16, fp32 PSUM):
  - level 0: the two props the einsum needs (Tx1[l0] = S0[l0]@X0 and
    p2t = S0[l0]@S0@X0) are composed on the HOST into single operators on
    the input X0 (2-hop edge expansion M0 = S0[l0]*S0). Edge-major X0 rows
    are pregathered host-side -> the props are pure streaming selection
    matmuls: zero indirect DMA, zero full-graph AllGather.
  - level 1: stacked dense operator T = [P_l1; S1[l1]; M1=S1[l1]*S1]
    (3072 x 4096) applied to z1, dest-sharded (128 l1-positions/core, all
    1024 batch-features wide); einsum is dest-sharded too. Comm: one
    chunked z1 AllGather + one small z2 AllGather.
  - levels 2-3: batch-sharded (4 batches/core), dense-S matmuls,
    block-diagonal channel mixes in feature-major layout.
  - MLP: output-feature sharded (512 cols of W6/7/8, 512 rows of W9 per
    core); activations [128k,32] are the stationary lhsT, W streams as rhs;
    BatchNorm per-feature after a PE transpose; activations AllGathered.
"""
import numpy as np
import concourse.bass as bass
import concourse.bacc as bacc
import concourse.tile as tile
from concourse import mybir, bass_utils

F32 = mybir.dt.float32
F16 = mybir.dt.float16
I16 = mybir.dt.int16
AF = mybir.ActivationFunctionType
ALU = mybir.AluOpType
AX = mybir.AxisListType
RG = [list(range(8))]
NCORES = 8
N0, N1, N2, N3 = 16384, 4096, 1024, 128
EPS = 1e-5
H16 = np.float16

_CACHE = {}


# ---------------------------------------------------------------- host prep
def _prep_prop(row, col, we, n_dest, n_shard):
    """Sorted-by-dest edges -> 128-dest windows, 128-edge chunks, padded so
    chunk counts per window match across shards (one SPMD program).
    Emits per-chunk selection matrices sel[chunk, edge_local, dst_local]."""
    window = 128
    order = np.argsort(row, kind="stable")
    row, col, we = row[order], col[order], we[order]
    per = n_dest // n_shard
    nwin = per // window
    counts = np.zeros((n_shard, nwin), np.int64)
    lists = {}
    for s in range(n_shard):
        lo = s * per
        for wi in range(nwin):
            wlo = lo + wi * window
            a = np.searchsorted(row, wlo, side="left")
            b = np.searchsorted(row, wlo + window, side="left")
            lists[(s, wi)] = (row[a:b] - wlo, col[a:b], we[a:b])
            counts[s, wi] = (b - a + 127) // 128
    ncw = np.maximum(counts.max(axis=0), 1)
    C = int(ncw.sum())
    src = np.zeros((n_shard, C, 128), np.int64)
    dst = np.full((n_shard, C, 128), 200.0, np.float32)
    wea = np.zeros((n_shard, C, 128), np.float32)
    for s in range(n_shard):
        base = 0
        for wi in range(nwin):
            dl, cl, wl = lists[(s, wi)]
            n = len(dl)
            k = int(ncw[wi])
            src[s, base:base + k].reshape(-1)[:n] = cl
            ch = base + np.arange(n) // 128
            ep = np.arange(n) % 128
            dst[s, ch, ep] = dl
            wea[s, ch, ep] = wl
            base += k
    return [int(x) for x in ncw], src, dst, wea


def _edge_we(e, n):
    row, col = np.asarray(e[0], np.int64), np.asarray(e[1], np.int64)
    deg = np.bincount(row, minlength=n).astype(np.float32)
    dis = np.where(deg > 0, 1.0 / np.sqrt(np.maximum(deg, 1.0)), 0.0).astype(np.float32)
    return row, col, -(dis[row] * dis[col]).astype(np.float32)


def _sub_edges(row, col, we, pool_idx):
    order = np.argsort(row, kind="stable")
    row, col, we = row[order], col[order], we[order]
    starts = np.searchsorted(row, pool_idx, side="left")
    ends = np.searchsorted(row, pool_idx, side="right")
    nr, ncl, nw = [], [], []
    for i in range(len(pool_idx)):
        s, e = starts[i], ends[i]
        if e > s:
            nr.append(np.full(e - s, i, np.int64))
            ncl.append(col[s:e])
            nw.append(we[s:e])
    return np.concatenate(nr), np.concatenate(ncl), np.concatenate(nw)


def _twohop(ri, ci, wi, row, col, we, n):
    """(i,j,w1) sub-edges composed with full edges (j->k,w2): (i,k,w1*w2)."""
    order = np.argsort(row, kind="stable")
    rs, cs, ws = row[order], col[order], we[order]
    starts = np.searchsorted(rs, np.arange(n), side="left")
    ends = np.searchsorted(rs, np.arange(n), side="right")
    cnt = (ends - starts)[ci]
    I = np.repeat(ri, cnt)
    W1 = np.repeat(wi, cnt)
    base = np.repeat(starts[ci], cnt)
    within = np.arange(cnt.sum()) - np.repeat(np.cumsum(cnt) - cnt, cnt)
    offs = base + within
    return I, cs[offs], W1 * ws[offs]


def _dense_s(row, col, we, n, m):
    s = np.zeros((n, m), np.float32)
    np.add.at(s, (row, col), we)
    return s


def _tile_w(w, pack):
    """[K, M] -> [K//(128*pack) * 128, pack*M]: pack K-blocks side by side."""
    k, m = w.shape
    nb = k // 128
    t = w.reshape(nb // pack, pack, 128, m).transpose(0, 2, 1, 3)
    return np.ascontiguousarray(t.reshape((nb // pack) * 128, pack * m))


def _idx_tile(flat):
    """flat int idx list -> [128, len//16] int16 (16-part wrap, x8 replicas)."""
    return np.ascontiguousarray(
        np.tile(flat.astype(np.int16).reshape(-1, 16).T, (8, 1)))


def _chunk_tile(arr3):
    """[C, 128, W] -> [128, C*W] (chunk c at cols c*W..)."""
    C, _, W = arr3.shape
    return np.ascontiguousarray(
        arr3.transpose(1, 0, 2).reshape(128, C * W)).astype(H16)


def _host_prep(inputs):
    d = {k: np.asarray(v) for k, v in inputs.items()}
    x = d["x"].astype(np.float32)
    l0 = np.asarray(d["l0"], np.int64)
    l1 = np.asarray(d["l1"], np.int64)
    l2 = np.asarray(d["l2"], np.int64)

    X0 = np.ascontiguousarray(x.transpose(1, 0, 2).reshape(N0, 96))
    X0p = np.zeros((N0, 128), np.float32)
    X0p[:, :96] = X0
    X0l0T = np.ascontiguousarray(X0[l0].T)  # [96, 4096]

    # level-0 operators on X0: a = S0[l0] (tap1), m = S0[l0]@S0 (tap2)
    r0, c0, w0 = _edge_we(d["e0"], N0)
    r0s, c0s, w0s = _sub_edges(r0, c0, w0, l0)
    ncw_a, src_a, dst_a, we_a = _prep_prop(r0s, c0s, w0s, N1, NCORES)
    mI, mK, mW = _twohop(r0s, c0s, w0s, r0, c0, w0, N0)
    ncw_m, src_m, dst_m, we_m = _prep_prop(mI, mK, mW, N1, NCORES)

    # level-1 stacked operator T = [P_l1; S1[l1]; M1]
    r1, c1, w1 = _edge_we(d["e1"], N1)
    S1 = _dense_s(r1, c1, w1, N1, N1)
    r1s, c1s, w1s = _sub_edges(r1, c1, w1, l1)
    S1l1 = _dense_s(r1s, c1s, w1s, N2, N1)    # [1024, 4096]
    M1 = S1l1 @ S1                            # [1024, 4096]
    P_l1 = np.zeros((N2, N1), np.float32)
    P_l1[np.arange(N2), l1] = 1.0
    Tblocks = [P_l1, S1l1, M1]

    r2, c2, w2 = _edge_we(d["e2"], N2)
    S2 = _dense_s(r2, c2, w2, N2, N2)
    S2T = _tile_w(np.ascontiguousarray(S2.T), 8).astype(H16)       # [128, 8192]
    S2l2T = _tile_w(np.ascontiguousarray(S2[l2].T), 8).astype(H16)  # [128, 1024]
    P_l2 = np.zeros((N2, 128), np.float32)
    P_l2[l2, np.arange(128)] = 1.0
    P_l2 = _tile_w(P_l2, 8).astype(H16)                             # [128, 1024]

    r3, c3, w3 = _edge_we(d["e3"], N3)
    S3T = np.ascontiguousarray(_dense_s(r3, c3, w3, N3, N3).T).astype(H16)

    def wmod(W):
        return W[0] - W[2], W[1], 2.0 * W[2]

    Wm1 = wmod(d["Wc1"].astype(np.float32))
    Wm = [wmod(d[f"Wc{i}"].astype(np.float32)) for i in (2, 3, 4, 5)]
    eye4 = np.eye(4, dtype=np.float32)

    per_core = []
    for k in range(NCORES):
        m = {}
        m["identbf"] = np.eye(128, dtype=np.float32).astype(H16)
        m["iota"] = np.tile(np.arange(128, dtype=np.float32), (128, 1))
        m["epsv"] = np.full((128, 1), EPS, np.float32)
        m["warm"] = np.zeros((1, 8), np.float32)
        m["X0l0Tw"] = np.ascontiguousarray(
            X0l0T[:, 512 * k:512 * (k + 1)]).astype(H16)
        m["Xg_a"] = _chunk_tile(X0p[src_a[k]].astype(H16))
        m["a_dst"] = np.ascontiguousarray(dst_a[k].T)
        m["a_we"] = np.ascontiguousarray(we_a[k].T)
        m["Xg_m"] = _chunk_tile(X0p[src_m[k]].astype(H16))
        m["m_dst"] = np.ascontiguousarray(dst_m[k].T)
        m["m_we"] = np.ascontiguousarray(we_m[k].T)
        # stacked-T lhsT chunks: block b, k-chunk kk at cols (b*32+kk)*128
        tt = np.zeros((128, 96 * 128), np.float32)
        for b, blk in enumerate(Tblocks):
            bt = blk[128 * k:128 * (k + 1), :].T  # [4096, 128]
            for kk in range(32):
                tt[:, (b * 32 + kk) * 128:(b * 32 + kk + 1) * 128] = \
                    bt[128 * kk:128 * (kk + 1), :]
        m["Tt"] = tt.astype(H16)
        m["S2T"] = S2T
        m["S2l2T"] = S2l2T
        m["P_l2"] = P_l2
        m["S3T"] = S3T
        for g in range(8):
            for t in range(3):
                bw = np.zeros((96, 128), np.float32)
                for j in range(4):
                    bg = 4 * g + j
                    bw[3 * bg:3 * bg + 3, 32 * j:32 * j + 32] = Wm1[t]
                m[f"bigw0_{g}_{t}"] = bw.astype(H16)
        for lev in range(4):
            for t in range(3):
                m[f"bigw{lev + 1}_{t}"] = np.kron(eye4, Wm[lev][t]).astype(H16)
        for lev, nm in ((1, "b1"), (2, "b2"), (3, "b3"), (4, "b4"), (5, "b5")):
            m[f"bias{lev}"] = np.tile(d[nm].astype(np.float32), 4).reshape(128, 1)
        for li in (6, 7, 8):
            W = d[f"W{li}"].astype(np.float32)[:, 512 * k:512 * k + 512]
            m[f"w{li}"] = np.ascontiguousarray(
                W.reshape(32, 128, 512).transpose(1, 0, 2).reshape(128, 32 * 512)
            ).astype(H16)
            m[f"g{li}"] = np.ascontiguousarray(
                d[f"g{li}"].astype(np.float32)[512 * k:512 * k + 512].reshape(4, 128).T)
            m[f"be{li}"] = np.ascontiguousarray(
                d[f"be{li}"].astype(np.float32)[512 * k:512 * k + 512].reshape(4, 128).T)
        W9 = d["W9"].astype(np.float32)[512 * k:512 * k + 512]  # [512, 128]
        m["w9"] = np.ascontiguousarray(
            W9.reshape(4, 128, 128).transpose(1, 0, 2).reshape(128, 512)).astype(H16)
        per_core.append(m)

    meta = {"a": ncw_a, "m": ncw_m}
    return per_core, meta


# ---------------------------------------------------------------- device program
def _build_nc(meta, shapes):
    nc = bacc.Bacc("TRN2", target_bir_lowering=False, debug=False, num_devices=NCORES)
    ein = {}
    for name, arr in shapes.items():
        dt = {np.dtype(np.int16): I16, np.dtype(H16): F16,
              np.dtype(np.float32): F32}[arr.dtype]
        ein[name] = nc.dram_tensor(name, list(arr.shape), dt, kind="ExternalInput")
    out_mu = nc.dram_tensor("mu", [128, 32], F32, kind="ExternalOutput")

    warm_all = nc.dram_tensor("warm_all", [8, 8], F32)
    warm_loc = nc.dram_tensor("warm_loc", [1, 8], F32)
    z1c = [nc.dram_tensor(f"z1c_{i}", [128, 1024], F16) for i in range(4)]
    z1ag = [nc.dram_tensor(f"z1ag_{i}", [1024, 1024], F16) for i in range(4)]
    z2T_loc = nc.dram_tensor("z2T_loc", [1024, 128], F16)
    z2a2a = nc.dram_tensor("z2a2a", [1024, 128], F16)
    x_loc = nc.dram_tensor("x_loc", [4, 4096], F16)
    x_all = nc.dram_tensor("x_all", [32, 4096], F16)
    h6_loc = nc.dram_tensor("h6_loc", [128, 128], F16)
    h6_all = nc.dram_tensor("h6_all", [1024, 128], F16)
    h7_loc = nc.dram_tensor("h7_loc", [128, 128], F16)
    h7_all = nc.dram_tensor("h7_all", [1024, 128], F16)
    p9_loc = nc.dram_tensor("p9_loc", [32, 128], F32)
    p9_all = nc.dram_tensor("p9_all", [256, 128], F32)

    def ag(loc_ap, all_ap):
        nc.gpsimd.collective_compute(
            "AllGather", ALU.bypass, replica_groups=RG,
            ins=[loc_ap.opt()], outs=[all_ap.opt()])

    with tile.TileContext(nc) as tc:
        with (
            tc.tile_pool(name="const", bufs=1) as cpool,
            tc.tile_pool(name="big", bufs=1) as bigpool,
            tc.tile_pool(name="work", bufs=3) as wpool,
            tc.tile_pool(name="wload", bufs=2) as wlpool,
            tc.tile_pool(name="psA", bufs=3, space="PSUM") as ppool,
            tc.tile_pool(name="psT", bufs=2, space="PSUM") as tpool,
            tc.tile_pool(name="psB", bufs=1, space="PSUM") as apool,
        ):
            ident = cpool.tile([128, 128], F16, tag="identbf", name="identbf")
            nc.sync.dma_start(out=ident[:], in_=ein["identbf"][:, :])
            iota_t = cpool.tile([128, 128], F32, tag="iota", name="iota")
            nc.sync.dma_start(out=iota_t[:], in_=ein["iota"][:, :])
            eps_t = cpool.tile([128, 1], F32, tag="epsv", name="epsv")
            nc.sync.dma_start(out=eps_t[:], in_=ein["epsv"][:, :])

            def load_const(name, dt=F16):
                t = cpool.tile(list(shapes[name].shape), dt, tag=name)
                nc.sync.dma_start(out=t[:], in_=ein[name][:, :])
                return t

            GRP = 16

            def grp_load(pref, g0, gc, tag, eng=None, grp=None):
                sl = wpool.tile([128, (grp or GRP) * 128], F16, tag=tag,
                                name=tag, bufs=2)
                (eng or nc.sync).dma_start(out=sl[:, :gc * 128],
                                           in_=ein[pref][:, g0 * 128:(g0 + gc) * 128])
                return sl

            def transp(src_ap, dst_ap):
                p, f = src_ap.shape
                ps = tpool.tile([128, 128], F16, tag="tp", name="tp")
                nc.tensor.transpose(out=ps[:f, :p], in_=src_ap, identity=ident[:p, :p])
                nc.scalar.activation(out=dst_ap, in_=ps[:f, :p], func=AF.Copy)

            def einsum_win(bigw, taps, Din, width, out_ap, func, bias_ap):
                ps = ppool.tile([128, 512], F32, tag="ps", name="ps")
                for t in range(3):
                    nc.tensor.matmul(out=ps[:, :width], lhsT=bigw[t][:Din, :],
                                     rhs=taps[t], start=(t == 0), stop=(t == 2))
                f2 = AF.Identity if func == AF.Copy else func
                nc.scalar.activation(out=out_ap, in_=ps[:, :width], func=f2, bias=bias_ap)

            # warm up the CC ring while level-0 computes
            with nc.named_scope("warmup"):
                warm = wpool.tile([1, 8], F32, tag="warm", name="warm")
                nc.sync.dma_start(out=warm[:], in_=ein["warm"][:, :])
                nc.sync.dma_start(out=warm_loc.ap(), in_=warm[:])
                ag(warm_loc.ap(), warm_all.ap())

            # ====== LEVEL 0: per-window pipeline of props -> einsum -> AG ====
            # tap1 = S0[l0]@X0, tap2 = (S0[l0]@S0)@X0, then the channel-mix
            # einsum for window wi immediately, then AllGather that window.
            with nc.named_scope("l0"):
                Ca, Cm = sum(meta["a"]), sum(meta["m"])
                cura = {"g0": -1}
                curm = {"g0": -1}

                def get_grp(cur, cc, C, xg, xtag, grp):
                    g0 = (cc // grp) * grp
                    if g0 != cur["g0"]:
                        gc = min(grp, C - g0)
                        cur["g0"] = g0
                        cur["x"] = grp_load(xg, g0, gc, xtag, eng=nc.scalar, grp=grp)
                    return cur, (cc - cur["g0"]) * 128

                def mk_sel(dw_t, cc):
                    sel = wpool.tile([128, 128], F16, tag="sel", name="sel", bufs=4)
                    nc.vector.tensor_scalar(
                        out=sel[:], in0=iota_t[:], scalar1=dw_t[0][:, cc:cc + 1],
                        scalar2=dw_t[1][:, cc:cc + 1], op0=ALU.is_equal, op1=ALU.mult)
                    return sel

                adw = [load_const("a_dst", F32), load_const("a_we", F32)]
                mdw = [load_const("m_dst", F32), load_const("m_we", F32)]

                bias1 = load_const("bias1", F32)
                x0w = cpool.tile([96, 512], F16, tag="X0l0Tw", name="X0l0Tw")
                nc.sync.dma_start(out=x0w[:], in_=ein["X0l0Tw"][:, :])
                bw0g = [[load_const(f"bigw0_{g}_{t}") for t in range(3)]
                        for g in range(8)]
                tap1_sb = bigpool.tile([96, 512], F16, tag="tap1_sb", name="tap1_sb")
                tap2_sb = bigpool.tile([96, 512], F16, tag="tap2_sb", name="tap2_sb")
                ba, bm = 0, 0
                for wi in range(4):
                    psa = ppool.tile([128, 512], F32, tag="ps", name="ps")
                    for c in range(meta["a"][wi]):
                        g, lo = get_grp(cura, ba + c, Ca, "Xg_a", "xga", 8)
                        nc.tensor.matmul(out=psa[:96, :128], lhsT=g["x"][:, lo:lo + 96],
                                         rhs=mk_sel(adw, ba + c)[:],
                                         start=(c == 0), stop=(c == meta["a"][wi] - 1))
                    nc.scalar.activation(out=tap1_sb[:, wi * 128:(wi + 1) * 128],
                                         in_=psa[:96, :128], func=AF.Copy)
                    ba += meta["a"][wi]
                    psm = ppool.tile([128, 512], F32, tag="ps", name="ps")
                    for c in range(meta["m"][wi]):
                        g, lo = get_grp(curm, bm + c, Cm, "Xg_m", "xgm", 16)
                        nc.tensor.matmul(out=psm[:96, :128], lhsT=g["x"][:, lo:lo + 96],
                                         rhs=mk_sel(mdw, bm + c)[:],
                                         start=(c == 0), stop=(c == meta["m"][wi] - 1))
                    nc.scalar.activation(out=tap2_sb[:, wi * 128:(wi + 1) * 128],
                                         in_=psm[:96, :128], func=AF.Copy)
                    bm += meta["m"][wi]
                    for g in range(8):
                        z1gT = wpool.tile([128, 128], F16, tag="z1Tw", name="z1Tw")
                        einsum_win(bw0g[g],
                                   [x0w[:, wi * 128:(wi + 1) * 128],
                                    tap1_sb[:96, wi * 128:(wi + 1) * 128],
                                    tap2_sb[:96, wi * 128:(wi + 1) * 128]],
                                   96, 128, z1gT[:], AF.Copy, bias1[:, 0:1])
                        t = wpool.tile([128, 128], F16, tag="z1nc", name="z1nc")
                        transp(z1gT[:], t[:])
                        nc.sync.dma_start(
                            out=z1c[wi][:, 128 * g:128 * (g + 1)], in_=t[:])
                    with nc.named_scope(f"agz1_{wi}"):
                        ag(z1c[wi].ap(), z1ag[wi].ap())

            # ====== LEVEL 1: taps = T @ z1, dest-sharded (128 pos x 1024) ====
            with nc.named_scope("l1_T"):
                tt = cpool.tile([128, 96 * 128], F16, tag="Tt", name="Tt")
                nc.sync.dma_start(out=tt[:, :6144], in_=ein["Tt"][:, :6144])
                nc.sync.dma_start(out=tt[:, 6144:], in_=ein["Tt"][:, 6144:])
                accs = [bigpool.tile([128, 1024], F16, tag=f"accT{b}", name=f"accT{b}")
                        for b in range(3)]
                for part in range(4):
                    for h in range(2):
                        z1sc = wpool.tile([128, 4096], F16, tag="z1sc", name="z1sc", bufs=2)
                        nc.sync.dma_start(
                            out=z1sc[:].rearrange("p (q d) -> p q d", d=1024),
                            in_=z1ag[part].ap()[512 * h:512 * (h + 1)]
                                .rearrange("(q p) d -> p q d", p=128))
                        for b in range(3):
                            psa = ppool.tile([128, 512], F32, tag="ps", name="ps")
                            psb = ppool.tile([128, 512], F32, tag="ps", name="ps")
                            for q in range(4):
                                kk = 4 * (4 * h + q) + part
                                lh = tt[:, (b * 32 + kk) * 128:(b * 32 + kk + 1) * 128]
                                nc.tensor.matmul(out=psa[:, :512], lhsT=lh,
                                                 rhs=z1sc[:, q * 1024:q * 1024 + 512],
                                                 start=(q == 0), stop=(q == 3))
                                nc.tensor.matmul(out=psb[:, :512], lhsT=lh,
                                                 rhs=z1sc[:, q * 1024 + 512:(q + 1) * 1024],
                                                 start=(q == 0), stop=(q == 3))
                            if part == 0 and h == 0:
                                nc.scalar.activation(out=accs[b][:, :512], in_=psa[:, :512],
                                                     func=AF.Copy)
                                nc.scalar.activation(out=accs[b][:, 512:], in_=psb[:, :512],
                                                     func=AF.Copy)
                            else:
                                nc.vector.tensor_add(accs[b][:, :512], accs[b][:, :512],
                                                     psa[:, :512])
                                nc.vector.tensor_add(accs[b][:, 512:], accs[b][:, 512:],
                                                     psb[:, :512])

            # ============ LEVEL 1 einsum (dest-sharded) -> z2T ===============
            with nc.named_scope("l1_einsum"):
                bw1 = [load_const(f"bigw1_{t}") for t in range(3)]
                bias2 = load_const("bias2", F32)
                tapTs = []
                for b in range(3):
                    tapT = bigpool.tile([128, 1024], F16, tag=f"tapT{b}", name=f"tapT{b}")
                    for f in range(8):
                        transp(accs[b][:, 128 * f:128 * (f + 1)],
                               tapT[:, 128 * f:128 * (f + 1)])
                    tapTs.append(tapT)
                for fg in range(8):
                    z2fg = wpool.tile([128, 128], F16, tag="z2fg", name="z2fg")
                    einsum_win(bw1, [tapTs[0][:, 128 * fg:128 * (fg + 1)],
                                     tapTs[1][:, 128 * fg:128 * (fg + 1)],
                                     tapTs[2][:, 128 * fg:128 * (fg + 1)]],
                               128, 128, z2fg[:], AF.Tanh, bias2[:, 0:1])
                    nc.sync.dma_start(out=z2T_loc[128 * fg:128 * (fg + 1), :],
                                      in_=z2fg[:])
            with nc.named_scope("a2a_z2"):
                nc.gpsimd.collective_compute(
                    "AllToAll", ALU.bypass, replica_groups=RG,
                    ins=[z2T_loc.ap().opt()], outs=[z2a2a.ap().opt()])

            # ====== z2n assembly (batch-sharded node-major) ==================
            z2n = bigpool.tile([128, 8 * 128], F16, tag="z2n", name="z2n")
            with nc.named_scope("z2n_asm"):
                zb = wpool.tile([128, 1024], F16, tag="zb", name="zb", bufs=1)
                for ci in range(8):
                    nc.sync.dma_start(out=zb[:, ci * 128:(ci + 1) * 128],
                                      in_=z2a2a[128 * ci:128 * (ci + 1), :])
                for ci in range(8):
                    transp(zb[:, ci * 128:(ci + 1) * 128],
                           z2n[:, ci * 128:(ci + 1) * 128])

            # ================= LEVEL 2 (dense) =================
            with nc.named_scope("l2"):
                t1_l2 = bigpool.tile([128, 8 * 128], F16, tag="t1_l2", name="t1_l2")
                for half in range(2):
                    s2t = wlpool.tile([128, 4096], F16, tag="wld", name="wld")
                    nc.sync.dma_start(out=s2t[:], in_=ein["S2T"][:, 4096 * half:4096 * (half + 1)])
                    for dc in range(8):
                        ps = ppool.tile([128, 512], F32, tag="ps", name="ps")
                        for kk in range(4):
                            kc = half * 4 + kk
                            nc.tensor.matmul(
                                out=ps[:, :128],
                                lhsT=s2t[:, kk * 1024 + dc * 128: kk * 1024 + dc * 128 + 128],
                                rhs=z2n[:, kc * 128:(kc + 1) * 128],
                                start=(kk == 0), stop=(kk == 3))
                        if half == 0:
                            nc.scalar.activation(out=t1_l2[:, dc * 128:(dc + 1) * 128],
                                                 in_=ps[:, :128], func=AF.Copy)
                        else:
                            nc.vector.tensor_add(t1_l2[:, dc * 128:(dc + 1) * 128],
                                                 t1_l2[:, dc * 128:(dc + 1) * 128],
                                                 ps[:, :128])
                s2l2 = cpool.tile([128, 1024], F16, tag="s2l2", name="s2l2")
                nc.sync.dma_start(out=s2l2[:], in_=ein["S2l2T"][:, :])
                ps = ppool.tile([128, 512], F32, tag="ps", name="ps")
                for kc in range(8):
                    nc.tensor.matmul(out=ps[:, :128], lhsT=s2l2[:, kc * 128:(kc + 1) * 128],
                                     rhs=t1_l2[:, kc * 128:(kc + 1) * 128],
                                     start=(kc == 0), stop=(kc == 7))
                p2n_l2 = wpool.tile([128, 128], F16, tag="p2n_l2", name="p2n_l2")
                nc.scalar.activation(out=p2n_l2[:], in_=ps[:, :128], func=AF.Copy)
                pl2 = cpool.tile([128, 1024], F16, tag="pl2", name="pl2")
                nc.sync.dma_start(out=pl2[:], in_=ein["P_l2"][:, :])
                z2l2T = wpool.tile([128, 128], F16, tag="z2l2T", name="z2l2T")
                psg = ppool.tile([128, 512], F32, tag="ps", name="ps")
                for kc in range(8):
                    nc.tensor.matmul(out=psg[:, :128], lhsT=z2n[:, kc * 128:(kc + 1) * 128],
                                     rhs=pl2[:, kc * 128:(kc + 1) * 128],
                                     start=(kc == 0), stop=(kc == 7))
                nc.scalar.activation(out=z2l2T[:], in_=psg[:, :128], func=AF.Copy)
                t1l2T = wpool.tile([128, 128], F16, tag="t1l2T", name="t1l2T")
                psg2 = ppool.tile([128, 512], F32, tag="ps", name="ps")
                for kc in range(8):
                    nc.tensor.matmul(out=psg2[:, :128], lhsT=t1_l2[:, kc * 128:(kc + 1) * 128],
                                     rhs=pl2[:, kc * 128:(kc + 1) * 128],
                                     start=(kc == 0), stop=(kc == 7))
                nc.scalar.activation(out=t1l2T[:], in_=psg2[:, :128], func=AF.Copy)
                p2l2T = wpool.tile([128, 128], F16, tag="p2l2T", name="p2l2T")
                transp(p2n_l2[:], p2l2T[:])
                bw2 = [load_const(f"bigw2_{t}") for t in range(3)]
                bias3 = load_const("bias3", F32)
                z3T = wpool.tile([128, 128], F16, tag="z3T", name="z3T")
                einsum_win(bw2, [z2l2T[:], t1l2T[:], p2l2T[:]], 128, 128,
                           z3T[:], AF.Tanh, bias3[:, 0:1])
                z3n = wpool.tile([128, 128], F16, tag="z3n", name="z3n")
                transp(z3T[:], z3n[:])

            # ================= LEVEL 3 =================
            with nc.named_scope("l3"):
                s3t = cpool.tile([128, 128], F16, tag="s3t", name="s3t")
                nc.sync.dma_start(out=s3t[:], in_=ein["S3T"][:, :])
                bias4 = load_const("bias4", F32)
                bias5 = load_const("bias5", F32)

                def conv_l3(zn, zT, bw_pref, bias_t, func, keep):
                    t1T = wpool.tile([128, 128], F16, tag=keep + "t1T", name=keep + "t1T")
                    ps = ppool.tile([128, 512], F32, tag="ps", name="ps")
                    nc.tensor.matmul(out=ps[:, :128], lhsT=zn, rhs=s3t[:], start=True, stop=True)
                    nc.scalar.activation(out=t1T[:], in_=ps[:, :128], func=AF.Copy)
                    t1n_ = wpool.tile([128, 128], F16, tag=keep + "t1n", name=keep + "t1n")
                    transp(t1T[:], t1n_[:])
                    p2T_ = wpool.tile([128, 128], F16, tag=keep + "p2T", name=keep + "p2T")
                    ps2 = ppool.tile([128, 512], F32, tag="ps", name="ps")
                    nc.tensor.matmul(out=ps2[:, :128], lhsT=t1n_[:], rhs=s3t[:], start=True, stop=True)
                    nc.scalar.activation(out=p2T_[:], in_=ps2[:, :128], func=AF.Copy)
                    bw = [load_const(f"{bw_pref}_{t}") for t in range(3)]
                    outT = wpool.tile([128, 128], F16, tag=keep + "oT", name=keep + "oT")
                    einsum_win(bw, [zT, t1T[:], p2T_[:]], 128, 128, outT[:], func, bias_t[:, 0:1])
                    outn = wpool.tile([128, 128], F16, tag=keep + "on", name=keep + "on")
                    transp(outT[:], outn[:])
                    return outn, outT

                z4n, z4T = conv_l3(z3n[:], z3T[:], "bigw3", bias4, AF.Tanh, "c4")
                o5n, o5T = conv_l3(z4n[:], z4T[:], "bigw4", bias5, AF.Copy, "c5")

            # ================= MLP input assembly =================
            with nc.named_scope("mlp_in"):
                nc.sync.dma_start(
                    out=x_loc.ap().rearrange("b (n c) -> n b c", c=32),
                    in_=o5n[:].rearrange("n (b c) -> n b c", c=32))
                ag(x_loc.ap(), x_all.ap())
                xT_sb = bigpool.tile([32, 4096], F16, tag="xT_sb", name="xT_sb")
                nc.sync.dma_start(out=xT_sb[:], in_=x_all[:, :])
                act6 = bigpool.tile([128, 1024], F16, tag="act6", name="act6")
                for i in range(32):
                    transp(xT_sb[:, 128 * i:128 * (i + 1)], act6[:, 32 * i:32 * i + 32])

            # ================= MLP =================
            def mlp_layer(li, act_sb, out_sb):
                g_t = load_const(f"g{li}", F32)
                be_t = load_const(f"be{li}", F32)
                wt = wlpool.tile([128, 32 * 512], F16, tag="wld", name="wld")
                nc.sync.dma_start(out=wt[:, :8192], in_=ein[f"w{li}"][:, :8192])
                nc.sync.dma_start(out=wt[:, 8192:], in_=ein[f"w{li}"][:, 8192:])
                acc = apool.tile([128, 512], F32, tag="acc", name="acc")
                for k2 in range(32):
                    nc.tensor.matmul(out=acc[:32, :], lhsT=act_sb[:, 32 * k2:32 * k2 + 32],
                                     rhs=wt[:, 512 * k2:512 * (k2 + 1)],
                                     start=(k2 == 0), stop=(k2 == 31))
                hb = wpool.tile([32, 512], F16, tag="hb", name="hb")
                nc.scalar.activation(out=hb[:], in_=acc[:32, :], func=AF.Copy)
                for c in range(4):
                    hc = wpool.tile([128, 32], F16, tag="hc", name="hc")
                    transp(hb[:, 128 * c:128 * (c + 1)], hc[:])
                    st6 = wpool.tile([128, 6], F32, tag="b_st6", name="b_st6")
                    nc.vector.bn_stats(out=st6[:], in_=hc[:])
                    mv = wpool.tile([128, 2], F32, tag="b_mv", name="b_mv")
                    nc.vector.bn_aggr(out=mv[:], in_=st6[:])
                    sd = wpool.tile([128, 1], F32, tag="b_sd", name="b_sd")
                    nc.scalar.activation(out=sd[:], in_=mv[:, 1:2], func=AF.Sqrt,
                                         bias=eps_t[:, 0:1])
                    rs = wpool.tile([128, 1], F32, tag="b_rs", name="b_rs")
                    nc.vector.reciprocal(rs[:], sd[:])
                    a_ = wpool.tile([128, 1], F32, tag="b_a", name="b_a")
                    nc.vector.tensor_mul(a_[:], rs[:], g_t[:, c:c + 1])
                    sh = wpool.tile([128, 1], F32, tag="b_sh", name="b_sh")
                    nc.vector.scalar_tensor_tensor(out=sh[:], in0=mv[:, 0:1], scalar=-1.0,
                                                   in1=a_[:], op0=ALU.mult, op1=ALU.mult)
                    nc.vector.tensor_add(sh[:], sh[:], be_t[:, c:c + 1])
                    nc.scalar.activation(out=out_sb[:, 32 * c:32 * c + 32], in_=hc[:],
                                         func=AF.Relu, scale=a_[:, 0:1], bias=sh[:, 0:1])

            with nc.named_scope("mlp6"):
                h6 = bigpool.tile([128, 128], F16, tag="h6sb", name="h6sb")
                mlp_layer(6, act6, h6)
                nc.sync.dma_start(out=h6_loc.ap(), in_=h6[:])
                ag(h6_loc.ap(), h6_all.ap())
            with nc.named_scope("mlp7"):
                act7 = bigpool.tile([128, 1024], F16, tag="act7", name="act7")
                for r in range(8):
                    nc.sync.dma_start(out=act7[:, 128 * r:128 * (r + 1)],
                                      in_=h6_all[128 * r:128 * (r + 1), :])
                h7 = bigpool.tile([128, 128], F16, tag="h7sb", name="h7sb")
                mlp_layer(7, act7, h7)
                nc.sync.dma_start(out=h7_loc.ap(), in_=h7[:])
                ag(h7_loc.ap(), h7_all.ap())
            with nc.named_scope("mlp8"):
                act8 = bigpool.tile([128, 1024], F16, tag="act8", name="act8")
                for r in range(8):
                    nc.sync.dma_start(out=act8[:, 128 * r:128 * (r + 1)],
                                      in_=h7_all[128 * r:128 * (r + 1), :])
                h8 = bigpool.tile([128, 128], F16, tag="h8sb", name="h8sb")
                mlp_layer(8, act8, h8)

            with nc.named_scope("mlp9"):
                w9t = cpool.tile([128, 512], F16, tag="w9t", name="w9t")
                nc.sync.dma_start(out=w9t[:], in_=ein["w9"][:, :])
                acc9 = apool.tile([128, 512], F32, tag="acc", name="acc9")
                for c in range(4):
                    nc.tensor.matmul(out=acc9[:32, :128], lhsT=h8[:, 32 * c:32 * c + 32],
                                     rhs=w9t[:, 128 * c:128 * (c + 1)],
                                     start=(c == 0), stop=(c == 3))
                p9sb = wpool.tile([32, 128], F32, tag="p9sb", name="p9sb")
                nc.scalar.activation(out=p9sb[:], in_=acc9[:32, :128], func=AF.Copy)
                nc.sync.dma_start(out=p9_loc.ap(), in_=p9sb[:])
                ag(p9_loc.ap(), p9_all.ap())
                tot = wpool.tile([32, 128], F32, tag="f_tot", name="f_tot")
                nc.sync.dma_start(out=tot[:], in_=p9_all[0:32, :])
                for k in range(1, 8):
                    pk = wpool.tile([32, 128], F32, tag="f_pk", name="f_pk")
                    nc.sync.dma_start(out=pk[:], in_=p9_all[32 * k:32 * (k + 1), :])
                    nc.vector.tensor_add(tot[:], tot[:], pk[:])
                totT = wpool.tile([128, 32], F32, tag="f_totT", name="f_totT")
                pst = ppool.tile([128, 512], F32, tag="ps", name="pst")
                identf = cpool.tile([32, 32], F32, tag="identf", name="identf")
                nc.scalar.activation(out=identf[:], in_=ident[:32, :32], func=AF.Copy)
                nc.tensor.transpose(out=pst[:128, :32], in_=tot[:], identity=identf[:])
                nc.scalar.activation(out=totT[:], in_=pst[:128, :32], func=AF.Copy)
                st6 = wpool.tile([128, 6], F32, tag="f_st6", name="f_st6")
                nc.vector.bn_stats(out=st6[:], in_=totT[:])
                mv = wpool.tile([128, 2], F32, tag="f_mv", name="f_mv")
                nc.vector.bn_aggr(out=mv[:], in_=st6[:])
                mu_ = mv[:, 0:1]
                sdf = wpool.tile([128, 1], F32, tag="f_sd", name="f_sd")
                nc.scalar.activation(out=sdf[:], in_=mv[:, 1:2], func=AF.Sqrt, bias=eps_t[:, 0:1])
                rs = wpool.tile([128, 1], F32, tag="f_rs", name="f_rs")
                nc.vector.reciprocal(rs[:], sdf[:])
                neg = wpool.tile([128, 1], F32, tag="f_neg", name="f_neg")
                nc.vector.scalar_tensor_tensor(out=neg[:], in0=mu_, scalar=-1.0,
                                               in1=rs[:], op0=ALU.mult, op1=ALU.mult)
                outt = wpool.tile([128, 32], F32, tag="f_out", name="f_out")
                nc.scalar.activation(out=outt[:], in_=totT[:], func=AF.Identity,
                                     scale=rs[:, 0:1], bias=neg[:, 0:1])
                nc.sync.dma_start(out=out_mu[:, :], in_=outt[:])

    nc.compile()
    return nc


# ---------------------------------------------------------------- entry point
def kernel(**inputs) -> np.ndarray:
    per_core, meta = _host_prep(inputs)
    if "prog" not in _CACHE:
        _CACHE["prog"] = _build_nc(meta, per_core[0])
    nc = _CACHE["prog"]
    res = bass_utils.run_bass_kernel_spmd(nc, per_core, core_ids=list(range(NCORES)))
    return np.ascontiguousarray(res.results[0]["mu"].T)


# revision 35
# speedup vs baseline: 3.5041x; 1.0277x over previous
"""Trainium2 Bass kernel for nn_Encoder_base (5x ChebConv GNN + pool + MLP).

Distribution over 8 NeuronCores (all matmuls fp16, fp32 PSUM):
  - level 0: the two props the einsum needs (Tx1[l0] = S0[l0]@X0 and
    p2t = S0[l0]@S0@X0) are composed on the HOST into single operators on
    the input X0 (2-hop edge expansion M0 = S0[l0]*S0). Edge-major X0 rows
    are pregathered host-side -> the props are pure streaming selection
    matmuls: zero indirect DMA, zero full-graph AllGather.
  - level 1: stacked dense operator T = [P_l1; S1[l1]; M1=S1[l1]*S1]
    (3072 x 4096) applied to z1, dest-sharded (128 l1-positions/core, all
    1024 batch-features wide); einsum is dest-sharded too. Comm: one
    chunked z1 AllGather + one small z2 AllGather.
  - levels 2-3: batch-sharded (4 batches/core), dense-S matmuls,
    block-diagonal channel mixes in feature-major layout.
  - MLP: output-feature sharded (512 cols of W6/7/8, 512 rows of W9 per
    core); activations [128k,32] are the stationary lhsT, W streams as rhs;
    BatchNorm per-feature after a PE transpose; activations AllGathered.
"""
import numpy as np
import concourse.bass as bass
import concourse.bacc as bacc
import concourse.tile as tile
from concourse import mybir, bass_utils

F32 = mybir.dt.float32
F16 = mybir.dt.float16
I16 = mybir.dt.int16
AF = mybir.ActivationFunctionType
ALU = mybir.AluOpType
AX = mybir.AxisListType
RG = [list(range(8))]
NCORES = 8
N0, N1, N2, N3 = 16384, 4096, 1024, 128
EPS = 1e-5
H16 = np.float16

_CACHE = {}


# ---------------------------------------------------------------- host prep
def _prep_prop(row, col, we, n_dest, n_shard):
    """Sorted-by-dest edges -> 128-dest windows, 128-edge chunks, padded so
    chunk counts per window match across shards (one SPMD program).
    Emits per-chunk selection matrices sel[chunk, edge_local, dst_local]."""
    window = 128
    order = np.argsort(row, kind="stable")
    row, col, we = row[order], col[order], we[order]
    per = n_dest // n_shard
    nwin = per // window
    counts = np.zeros((n_shard, nwin), np.int64)
    lists = {}
    for s in range(n_shard):
        lo = s * per
        for wi in range(nwin):
            wlo = lo + wi * window
            a = np.searchsorted(row, wlo, side="left")
            b = np.searchsorted(row, wlo + window, side="left")
            lists[(s, wi)] = (row[a:b] - wlo, col[a:b], we[a:b])
            counts[s, wi] = (b - a + 127) // 128
    ncw = np.maximum(counts.max(axis=0), 1)
    C = int(ncw.sum())
    src = np.zeros((n_shard, C, 128), np.int64)
    dst = np.full((n_shard, C, 128), 200.0, np.float32)
    wea = np.zeros((n_shard, C, 128), np.float32)
    for s in range(n_shard):
        base = 0
        for wi in range(nwin):
            dl, cl, wl = lists[(s, wi)]
            n = len(dl)
            k = int(ncw[wi])
            src[s, base:base + k].reshape(-1)[:n] = cl
            ch = base + np.arange(n) // 128
            ep = np.arange(n) % 128
            dst[s, ch, ep] = dl
            wea[s, ch, ep] = wl
            base += k
    return [int(x) for x in ncw], src, dst, wea


def _edge_we(e, n):
    row, col = np.asarray(e[0], np.int64), np.asarray(e[1], np.int64)
    deg = np.bincount(row, minlength=n).astype(np.float32)
    dis = np.where(deg > 0, 1.0 / np.sqrt(np.maximum(deg, 1.0)), 0.0).astype(np.float32)
    return row, col, -(dis[row] * dis[col]).astype(np.float32)


def _sub_edges(row, col, we, pool_idx):
    order = np.argsort(row, kind="stable")
    row, col, we = row[order], col[order], we[order]
    starts = np.searchsorted(row, pool_idx, side="left")
    ends = np.searchsorted(row, pool_idx, side="right")
    nr, ncl, nw = [], [], []
    for i in range(len(pool_idx)):
        s, e = starts[i], ends[i]
        if e > s:
            nr.append(np.full(e - s, i, np.int64))
            ncl.append(col[s:e])
            nw.append(we[s:e])
    return np.concatenate(nr), np.concatenate(ncl), np.concatenate(nw)


def _twohop(ri, ci, wi, row, col, we, n):
    """(i,j,w1) sub-edges composed with full edges (j->k,w2): (i,k,w1*w2)."""
    order = np.argsort(row, kind="stable")
    rs, cs, ws = row[order], col[order], we[order]
    starts = np.searchsorted(rs, np.arange(n), side="left")
    ends = np.searchsorted(rs, np.arange(n), side="right")
    cnt = (ends - starts)[ci]
    I = np.repeat(ri, cnt)
    W1 = np.repeat(wi, cnt)
    base = np.repeat(starts[ci], cnt)
    within = np.arange(cnt.sum()) - np.repeat(np.cumsum(cnt) - cnt, cnt)
    offs = base + within
    return I, cs[offs], W1 * ws[offs]


def _dense_s(row, col, we, n, m):
    s = np.zeros((n, m), np.float32)
    np.add.at(s, (row, col), we)
    return s


def _tile_w(w, pack):
    """[K, M] -> [K//(128*pack) * 128, pack*M]: pack K-blocks side by side."""
    k, m = w.shape
    nb = k // 128
    t = w.reshape(nb // pack, pack, 128, m).transpose(0, 2, 1, 3)
    return np.ascontiguousarray(t.reshape((nb // pack) * 128, pack * m))


def _idx_tile(flat):
    """flat int idx list -> [128, len//16] int16 (16-part wrap, x8 replicas)."""
    return np.ascontiguousarray(
        np.tile(flat.astype(np.int16).reshape(-1, 16).T, (8, 1)))


def _chunk_tile(arr3):
    """[C, 128, W] -> [128, C*W] (chunk c at cols c*W..)."""
    C, _, W = arr3.shape
    return np.ascontiguousarray(
        arr3.transpose(1, 0, 2).reshape(128, C * W)).astype(H16)


def _host_prep(inputs):
    d = {k: np.asarray(v) for k, v in inputs.items()}
    x = d["x"].astype(np.float32)
    l0 = np.asarray(d["l0"], np.int64)
    l1 = np.asarray(d["l1"], np.int64)
    l2 = np.asarray(d["l2"], np.int64)

    X0 = np.ascontiguousarray(x.transpose(1, 0, 2).reshape(N0, 96))
    X0p = np.zeros((N0, 128), np.float32)
    X0p[:, :96] = X0
    X0l0T = np.ascontiguousarray(X0[l0].T)  # [96, 4096]

    # level-0 operators on X0: a = S0[l0] (tap1), m = S0[l0]@S0 (tap2)
    r0, c0, w0 = _edge_we(d["e0"], N0)
    r0s, c0s, w0s = _sub_edges(r0, c0, w0, l0)
    ncw_a, src_a, dst_a, we_a = _prep_prop(r0s, c0s, w0s, N1, NCORES)
    mI, mK, mW = _twohop(r0s, c0s, w0s, r0, c0, w0, N0)
    ncw_m, src_m, dst_m, we_m = _prep_prop(mI, mK, mW, N1, NCORES)

    # level-1 stacked operator T = [P_l1; S1[l1]; M1]
    r1, c1, w1 = _edge_we(d["e1"], N1)
    S1 = _dense_s(r1, c1, w1, N1, N1)
    r1s, c1s, w1s = _sub_edges(r1, c1, w1, l1)
    S1l1 = _dense_s(r1s, c1s, w1s, N2, N1)    # [1024, 4096]
    M1 = S1l1 @ S1                            # [1024, 4096]
    P_l1 = np.zeros((N2, N1), np.float32)
    P_l1[np.arange(N2), l1] = 1.0
    Tblocks = [P_l1, S1l1, M1]

    r2, c2, w2 = _edge_we(d["e2"], N2)
    S2 = _dense_s(r2, c2, w2, N2, N2)
    S2T = _tile_w(np.ascontiguousarray(S2.T), 8).astype(H16)       # [128, 8192]
    S2l2T = _tile_w(np.ascontiguousarray(S2[l2].T), 8).astype(H16)  # [128, 1024]
    P_l2 = np.zeros((N2, 128), np.float32)
    P_l2[l2, np.arange(128)] = 1.0
    P_l2 = _tile_w(P_l2, 8).astype(H16)                             # [128, 1024]

    r3, c3, w3 = _edge_we(d["e3"], N3)
    S3T = np.ascontiguousarray(_dense_s(r3, c3, w3, N3, N3).T).astype(H16)

    def wmod(W):
        return W[0] - W[2], W[1], 2.0 * W[2]

    Wm1 = wmod(d["Wc1"].astype(np.float32))
    Wm = [wmod(d[f"Wc{i}"].astype(np.float32)) for i in (2, 3, 4, 5)]
    eye4 = np.eye(4, dtype=np.float32)

    per_core = []
    for k in range(NCORES):
        m = {}
        m["identbf"] = np.eye(128, dtype=np.float32).astype(H16)
        m["iota"] = np.tile(np.arange(128, dtype=np.float32), (128, 1))
        m["epsv"] = np.full((128, 1), EPS, np.float32)
        m["warm"] = np.zeros((1, 8), np.float32)
        m["X0l0Tw"] = np.ascontiguousarray(
            X0l0T[:, 512 * k:512 * (k + 1)]).astype(H16)
        m["Xg_a"] = _chunk_tile(X0p[src_a[k]].astype(H16))
        m["a_dst"] = np.ascontiguousarray(dst_a[k].T)
        m["a_we"] = np.ascontiguousarray(we_a[k].T)
        m["Xg_m"] = _chunk_tile(X0p[src_m[k]].astype(H16))
        m["m_dst"] = np.ascontiguousarray(dst_m[k].T)
        m["m_we"] = np.ascontiguousarray(we_m[k].T)
        # stacked-T lhsT chunks: block b, k-chunk kk at cols (b*32+kk)*128
        tt = np.zeros((128, 96 * 128), np.float32)
        for b, blk in enumerate(Tblocks):
            bt = blk[128 * k:128 * (k + 1), :].T  # [4096, 128]
            for kk in range(32):
                tt[:, (b * 32 + kk) * 128:(b * 32 + kk + 1) * 128] = \
                    bt[128 * kk:128 * (kk + 1), :]
        m["Tt"] = tt.astype(H16)
        m["S2T"] = S2T
        m["S2l2T"] = S2l2T
        m["P_l2"] = P_l2
        m["S3T"] = S3T
        for g in range(8):
            for t in range(3):
                bw = np.zeros((96, 128), np.float32)
                for j in range(4):
                    bg = 4 * g + j
                    bw[3 * bg:3 * bg + 3, 32 * j:32 * j + 32] = Wm1[t]
                m[f"bigw0_{g}_{t}"] = bw.astype(H16)
        for lev in range(4):
            for t in range(3):
                m[f"bigw{lev + 1}_{t}"] = np.kron(eye4, Wm[lev][t]).astype(H16)
        for lev, nm in ((1, "b1"), (2, "b2"), (3, "b3"), (4, "b4"), (5, "b5")):
            m[f"bias{lev}"] = np.tile(d[nm].astype(np.float32), 4).reshape(128, 1)
        for li in (6, 7, 8):
            W = d[f"W{li}"].astype(np.float32)[:, 512 * k:512 * k + 512]
            m[f"w{li}"] = np.ascontiguousarray(
                W.reshape(32, 128, 512).transpose(1, 0, 2).reshape(128, 32 * 512)
            ).astype(H16)
            m[f"g{li}"] = np.ascontiguousarray(
                d[f"g{li}"].astype(np.float32)[512 * k:512 * k + 512].reshape(4, 128).T)
            m[f"be{li}"] = np.ascontiguousarray(
                d[f"be{li}"].astype(np.float32)[512 * k:512 * k + 512].reshape(4, 128).T)
        W9 = d["W9"].astype(np.float32)[512 * k:512 * k + 512]  # [512, 128]
        m["w9"] = np.ascontiguousarray(
            W9.reshape(4, 128, 128).transpose(1, 0, 2).reshape(128, 512)).astype(H16)
        per_core.append(m)

    meta = {"a": ncw_a, "m": ncw_m}
    return per_core, meta


# ---------------------------------------------------------------- device program
def _build_nc(meta, shapes):
    nc = bacc.Bacc("TRN2", target_bir_lowering=False, debug=False, num_devices=NCORES)
    ein = {}
    for name, arr in shapes.items():
        dt = {np.dtype(np.int16): I16, np.dtype(H16): F16,
              np.dtype(np.float32): F32}[arr.dtype]
        ein[name] = nc.dram_tensor(name, list(arr.shape), dt, kind="ExternalInput")
    out_mu = nc.dram_tensor("mu", [128, 32], F32, kind="ExternalOutput")

    warm_all = nc.dram_tensor("warm_all", [8, 8], F32, addr_space="Shared")
    warm_loc = nc.dram_tensor("warm_loc", [1, 8], F32)
    z1c = [nc.dram_tensor(f"z1c_{i}", [128, 1024], F16) for i in range(4)]
    z1ag = [nc.dram_tensor(f"z1ag_{i}", [1024, 1024], F16, addr_space="Shared")
            for i in range(4)]
    z2T_loc = nc.dram_tensor("z2T_loc", [1024, 128], F16)
    z2a2a = nc.dram_tensor("z2a2a", [1024, 128], F16)
    x_loc = nc.dram_tensor("x_loc", [4, 4096], F16)
    x_all = nc.dram_tensor("x_all", [32, 4096], F16, addr_space="Shared")
    h6_loc = nc.dram_tensor("h6_loc", [128, 128], F16)
    h6_all = nc.dram_tensor("h6_all", [1024, 128], F16, addr_space="Shared")
    h7_loc = nc.dram_tensor("h7_loc", [128, 128], F16)
    h7_all = nc.dram_tensor("h7_all", [1024, 128], F16, addr_space="Shared")
    p9_loc = nc.dram_tensor("p9_loc", [32, 128], F32)
    p9_all = nc.dram_tensor("p9_all", [256, 128], F32, addr_space="Shared")

    def ag(loc_ap, all_ap):
        nc.gpsimd.collective_compute(
            "AllGather", ALU.bypass, replica_groups=RG,
            ins=[loc_ap.opt()], outs=[all_ap.opt()])

    with tile.TileContext(nc) as tc:
        with (
            tc.tile_pool(name="const", bufs=1) as cpool,
            tc.tile_pool(name="big", bufs=1) as bigpool,
            tc.tile_pool(name="work", bufs=3) as wpool,
            tc.tile_pool(name="wload", bufs=2) as wlpool,
            tc.tile_pool(name="psA", bufs=3, space="PSUM") as ppool,
            tc.tile_pool(name="psT", bufs=2, space="PSUM") as tpool,
            tc.tile_pool(name="psB", bufs=1, space="PSUM") as apool,
        ):
            ident = cpool.tile([128, 128], F16, tag="identbf", name="identbf")
            nc.sync.dma_start(out=ident[:], in_=ein["identbf"][:, :])
            iota_t = cpool.tile([128, 128], F32, tag="iota", name="iota")
            nc.sync.dma_start(out=iota_t[:], in_=ein["iota"][:, :])
            eps_t = cpool.tile([128, 1], F32, tag="epsv", name="epsv")
            nc.sync.dma_start(out=eps_t[:], in_=ein["epsv"][:, :])

            def load_const(name, dt=F16):
                t = cpool.tile(list(shapes[name].shape), dt, tag=name)
                nc.sync.dma_start(out=t[:], in_=ein[name][:, :])
                return t

            GRP = 16

            def grp_load(pref, g0, gc, tag, eng=None, grp=None):
                sl = wpool.tile([128, (grp or GRP) * 128], F16, tag=tag,
                                name=tag, bufs=2)
                (eng or nc.sync).dma_start(out=sl[:, :gc * 128],
                                           in_=ein[pref][:, g0 * 128:(g0 + gc) * 128])
                return sl

            def transp(src_ap, dst_ap):
                p, f = src_ap.shape
                ps = tpool.tile([128, 128], F16, tag="tp", name="tp")
                nc.tensor.transpose(out=ps[:f, :p], in_=src_ap, identity=ident[:p, :p])
                nc.scalar.activation(out=dst_ap, in_=ps[:f, :p], func=AF.Copy)

            def einsum_win(bigw, taps, Din, width, out_ap, func, bias_ap):
                ps = ppool.tile([128, 512], F32, tag="ps", name="ps")
                for t in range(3):
                    nc.tensor.matmul(out=ps[:, :width], lhsT=bigw[t][:Din, :],
                                     rhs=taps[t], start=(t == 0), stop=(t == 2))
                f2 = AF.Identity if func == AF.Copy else func
                nc.scalar.activation(out=out_ap, in_=ps[:, :width], func=f2, bias=bias_ap)

            # warm up the CC ring while level-0 computes
            with nc.named_scope("warmup"):
                ag(warm_loc.ap(), warm_all.ap())

            # ====== LEVEL 0: per-window pipeline of props -> einsum -> AG ====
            # tap1 = S0[l0]@X0, tap2 = (S0[l0]@S0)@X0, then the channel-mix
            # einsum for window wi immediately, then AllGather that window.
            with nc.named_scope("l0"):
                Ca, Cm = sum(meta["a"]), sum(meta["m"])
                cura = {"g0": -1}
                curm = {"g0": -1}

                def get_grp(cur, cc, C, xg, xtag, grp):
                    g0 = (cc // grp) * grp
                    if g0 != cur["g0"]:
                        gc = min(grp, C - g0)
                        cur["g0"] = g0
                        cur["x"] = grp_load(xg, g0, gc, xtag, eng=nc.scalar, grp=grp)
                    return cur, (cc - cur["g0"]) * 128

                def mk_sel(dw_t, cc):
                    sel = wpool.tile([128, 128], F16, tag="sel", name="sel", bufs=4)
                    nc.vector.tensor_scalar(
                        out=sel[:], in0=iota_t[:], scalar1=dw_t[0][:, cc:cc + 1],
                        scalar2=dw_t[1][:, cc:cc + 1], op0=ALU.is_equal, op1=ALU.mult)
                    return sel

                adw = [load_const("a_dst", F32), load_const("a_we", F32)]
                mdw = [load_const("m_dst", F32), load_const("m_we", F32)]

                bias1 = load_const("bias1", F32)
                x0w = cpool.tile([96, 512], F16, tag="X0l0Tw", name="X0l0Tw")
                nc.sync.dma_start(out=x0w[:], in_=ein["X0l0Tw"][:, :])
                bw0g = [[load_const(f"bigw0_{g}_{t}") for t in range(3)]
                        for g in range(8)]
                tap1_sb = bigpool.tile([96, 512], F16, tag="tap1_sb", name="tap1_sb")
                tap2_sb = bigpool.tile([96, 512], F16, tag="tap2_sb", name="tap2_sb")
                ba, bm = 0, 0
                for wi in range(4):
                    psa = ppool.tile([128, 512], F32, tag="ps", name="ps")
                    for c in range(meta["a"][wi]):
                        g, lo = get_grp(cura, ba + c, Ca, "Xg_a", "xga", 8)
                        nc.tensor.matmul(out=psa[:96, :128], lhsT=g["x"][:, lo:lo + 96],
                                         rhs=mk_sel(adw, ba + c)[:],
                                         start=(c == 0), stop=(c == meta["a"][wi] - 1))
                    nc.scalar.activation(out=tap1_sb[:, wi * 128:(wi + 1) * 128],
                                         in_=psa[:96, :128], func=AF.Copy)
                    ba += meta["a"][wi]
                    psm = ppool.tile([128, 512], F32, tag="ps", name="ps")
                    for c in range(meta["m"][wi]):
                        g, lo = get_grp(curm, bm + c, Cm, "Xg_m", "xgm", 16)
                        nc.tensor.matmul(out=psm[:96, :128], lhsT=g["x"][:, lo:lo + 96],
                                         rhs=mk_sel(mdw, bm + c)[:],
                                         start=(c == 0), stop=(c == meta["m"][wi] - 1))
                    nc.scalar.activation(out=tap2_sb[:, wi * 128:(wi + 1) * 128],
                                         in_=psm[:96, :128], func=AF.Copy)
                    bm += meta["m"][wi]
                    for g in range(8):
                        z1gT = wpool.tile([128, 128], F16, tag="z1Tw", name="z1Tw")
                        einsum_win(bw0g[g],
                                   [x0w[:, wi * 128:(wi + 1) * 128],
                                    tap1_sb[:96, wi * 128:(wi + 1) * 128],
                                    tap2_sb[:96, wi * 128:(wi + 1) * 128]],
                                   96, 128, z1gT[:], AF.Copy, bias1[:, 0:1])
                        t = wpool.tile([128, 128], F16, tag="z1nc", name="z1nc")
                        transp(z1gT[:], t[:])
                        nc.sync.dma_start(
                            out=z1c[wi][:, 128 * g:128 * (g + 1)], in_=t[:])
                    with nc.named_scope(f"agz1_{wi}"):
                        ag(z1c[wi].ap(), z1ag[wi].ap())

            # ====== LEVEL 1: taps = T @ z1, dest-sharded (128 pos x 1024) ====
            with nc.named_scope("l1_T"):
                tt = cpool.tile([128, 96 * 128], F16, tag="Tt", name="Tt")
                nc.sync.dma_start(out=tt[:, :6144], in_=ein["Tt"][:, :6144])
                nc.sync.dma_start(out=tt[:, 6144:], in_=ein["Tt"][:, 6144:])
                accs = [bigpool.tile([128, 1024], F16, tag=f"accT{b}", name=f"accT{b}")
                        for b in range(3)]
                for part in range(4):
                    for h in range(2):
                        z1sc = wpool.tile([128, 4096], F16, tag="z1sc", name="z1sc", bufs=2)
                        nc.sync.dma_start(
                            out=z1sc[:].rearrange("p (q d) -> p q d", d=1024),
                            in_=z1ag[part].ap()[512 * h:512 * (h + 1)]
                                .rearrange("(q p) d -> p q d", p=128))
                        for b in range(3):
                            psa = ppool.tile([128, 512], F32, tag="ps", name="ps")
                            psb = ppool.tile([128, 512], F32, tag="ps", name="ps")
                            for q in range(4):
                                kk = 4 * (4 * h + q) + part
                                lh = tt[:, (b * 32 + kk) * 128:(b * 32 + kk + 1) * 128]
                                nc.tensor.matmul(out=psa[:, :512], lhsT=lh,
                                                 rhs=z1sc[:, q * 1024:q * 1024 + 512],
                                                 start=(q == 0), stop=(q == 3))
                                nc.tensor.matmul(out=psb[:, :512], lhsT=lh,
                                                 rhs=z1sc[:, q * 1024 + 512:(q + 1) * 1024],
                                                 start=(q == 0), stop=(q == 3))
                            if part == 0 and h == 0:
                                nc.scalar.activation(out=accs[b][:, :512], in_=psa[:, :512],
                                                     func=AF.Copy)
                                nc.scalar.activation(out=accs[b][:, 512:], in_=psb[:, :512],
                                                     func=AF.Copy)
                            else:
                                nc.vector.tensor_add(accs[b][:, :512], accs[b][:, :512],
                                                     psa[:, :512])
                                nc.vector.tensor_add(accs[b][:, 512:], accs[b][:, 512:],
                                                     psb[:, :512])

            # ============ LEVEL 1 einsum (dest-sharded) -> z2T ===============
            with nc.named_scope("l1_einsum"):
                bw1 = [load_const(f"bigw1_{t}") for t in range(3)]
                bias2 = load_const("bias2", F32)
                tapTs = []
                for b in range(3):
                    tapT = bigpool.tile([128, 1024], F16, tag=f"tapT{b}", name=f"tapT{b}")
                    for f in range(8):
                        transp(accs[b][:, 128 * f:128 * (f + 1)],
                               tapT[:, 128 * f:128 * (f + 1)])
                    tapTs.append(tapT)
                for fg in range(8):
                    z2fg = wpool.tile([128, 128], F16, tag="z2fg", name="z2fg")
                    einsum_win(bw1, [tapTs[0][:, 128 * fg:128 * (fg + 1)],
                                     tapTs[1][:, 128 * fg:128 * (fg + 1)],
                                     tapTs[2][:, 128 * fg:128 * (fg + 1)]],
                               128, 128, z2fg[:], AF.Tanh, bias2[:, 0:1])
                    nc.sync.dma_start(out=z2T_loc[128 * fg:128 * (fg + 1), :],
                                      in_=z2fg[:])
            with nc.named_scope("a2a_z2"):
                nc.gpsimd.collective_compute(
                    "AllToAll", ALU.bypass, replica_groups=RG,
                    ins=[z2T_loc.ap().opt()], outs=[z2a2a.ap().opt()])

            # ====== z2n assembly (batch-sharded node-major) ==================
            z2n = bigpool.tile([128, 8 * 128], F16, tag="z2n", name="z2n")
            with nc.named_scope("z2n_asm"):
                zb = wpool.tile([128, 1024], F16, tag="zb", name="zb", bufs=1)
                for ci in range(8):
                    nc.sync.dma_start(out=zb[:, ci * 128:(ci + 1) * 128],
                                      in_=z2a2a[128 * ci:128 * (ci + 1), :])
                for ci in range(8):
                    transp(zb[:, ci * 128:(ci + 1) * 128],
                           z2n[:, ci * 128:(ci + 1) * 128])

            # ================= LEVEL 2 (dense) =================
            with nc.named_scope("l2"):
                t1_l2 = bigpool.tile([128, 8 * 128], F16, tag="t1_l2", name="t1_l2")
                for half in range(2):
                    s2t = wlpool.tile([128, 4096], F16, tag="wld", name="wld")
                    nc.sync.dma_start(out=s2t[:], in_=ein["S2T"][:, 4096 * half:4096 * (half + 1)])
                    for dc in range(8):
                        ps = ppool.tile([128, 512], F32, tag="ps", name="ps")
                        for kk in range(4):
                            kc = half * 4 + kk
                            nc.tensor.matmul(
                                out=ps[:, :128],
                                lhsT=s2t[:, kk * 1024 + dc * 128: kk * 1024 + dc * 128 + 128],
                                rhs=z2n[:, kc * 128:(kc + 1) * 128],
                                start=(kk == 0), stop=(kk == 3))
                        if half == 0:
                            nc.scalar.activation(out=t1_l2[:, dc * 128:(dc + 1) * 128],
                                                 in_=ps[:, :128], func=AF.Copy)
                        else:
                            nc.vector.tensor_add(t1_l2[:, dc * 128:(dc + 1) * 128],
                                                 t1_l2[:, dc * 128:(dc + 1) * 128],
                                                 ps[:, :128])
                s2l2 = cpool.tile([128, 1024], F16, tag="s2l2", name="s2l2")
                nc.sync.dma_start(out=s2l2[:], in_=ein["S2l2T"][:, :])
                ps = ppool.tile([128, 512], F32, tag="ps", name="ps")
                for kc in range(8):
                    nc.tensor.matmul(out=ps[:, :128], lhsT=s2l2[:, kc * 128:(kc + 1) * 128],
                                     rhs=t1_l2[:, kc * 128:(kc + 1) * 128],
                                     start=(kc == 0), stop=(kc == 7))
                p2n_l2 = wpool.tile([128, 128], F16, tag="p2n_l2", name="p2n_l2")
                nc.scalar.activation(out=p2n_l2[:], in_=ps[:, :128], func=AF.Copy)
                pl2 = cpool.tile([128, 1024], F16, tag="pl2", name="pl2")
                nc.sync.dma_start(out=pl2[:], in_=ein["P_l2"][:, :])
                z2l2T = wpool.tile([128, 128], F16, tag="z2l2T", name="z2l2T")
                psg = ppool.tile([128, 512], F32, tag="ps", name="ps")
                for kc in range(8):
                    nc.tensor.matmul(out=psg[:, :128], lhsT=z2n[:, kc * 128:(kc + 1) * 128],
                                     rhs=pl2[:, kc * 128:(kc + 1) * 128],
                                     start=(kc == 0), stop=(kc == 7))
                nc.scalar.activation(out=z2l2T[:], in_=psg[:, :128], func=AF.Copy)
                t1l2T = wpool.tile([128, 128], F16, tag="t1l2T", name="t1l2T")
                psg2 = ppool.tile([128, 512], F32, tag="ps", name="ps")
                for kc in range(8):
                    nc.tensor.matmul(out=psg2[:, :128], lhsT=t1_l2[:, kc * 128:(kc + 1) * 128],
                                     rhs=pl2[:, kc * 128:(kc + 1) * 128],
                                     start=(kc == 0), stop=(kc == 7))
                nc.scalar.activation(out=t1l2T[:], in_=psg2[:, :128], func=AF.Copy)
                p2l2T = wpool.tile([128, 128], F16, tag="p2l2T", name="p2l2T")
                transp(p2n_l2[:], p2l2T[:])
                bw2 = [load_const(f"bigw2_{t}") for t in range(3)]
                bias3 = load_const("bias3", F32)
                z3T = wpool.tile([128, 128], F16, tag="z3T", name="z3T")
                einsum_win(bw2, [z2l2T[:], t1l2T[:], p2l2T[:]], 128, 128,
                           z3T[:], AF.Tanh, bias3[:, 0:1])
                z3n = wpool.tile([128, 128], F16, tag="z3n", name="z3n")
                transp(z3T[:], z3n[:])

            # ================= LEVEL 3 =================
            with nc.named_scope("l3"):
                s3t = cpool.tile([128, 128], F16, tag="s3t", name="s3t")
                nc.sync.dma_start(out=s3t[:], in_=ein["S3T"][:, :])
                bias4 = load_const("bias4", F32)
                bias5 = load_const("bias5", F32)

                def conv_l3(zn, zT, bw_pref, bias_t, func, keep):
                    t1T = wpool.tile([128, 128], F16, tag=keep + "t1T", name=keep + "t1T")
                    ps = ppool.tile([128, 512], F32, tag="ps", name="ps")
                    nc.tensor.matmul(out=ps[:, :128], lhsT=zn, rhs=s3t[:], start=True, stop=True)
                    nc.scalar.activation(out=t1T[:], in_=ps[:, :128], func=AF.Copy)
                    t1n_ = wpool.tile([128, 128], F16, tag=keep + "t1n", name=keep + "t1n")
                    transp(t1T[:], t1n_[:])
                    p2T_ = wpool.tile([128, 128], F16, tag=keep + "p2T", name=keep + "p2T")
                    ps2 = ppool.tile([128, 512], F32, tag="ps", name="ps")
                    nc.tensor.matmul(out=ps2[:, :128], lhsT=t1n_[:], rhs=s3t[:], start=True, stop=True)
                    nc.scalar.activation(out=p2T_[:], in_=ps2[:, :128], func=AF.Copy)
                    bw = [load_const(f"{bw_pref}_{t}") for t in range(3)]
                    outT = wpool.tile([128, 128], F16, tag=keep + "oT", name=keep + "oT")
                    einsum_win(bw, [zT, t1T[:], p2T_[:]], 128, 128, outT[:], func, bias_t[:, 0:1])
                    outn = wpool.tile([128, 128], F16, tag=keep + "on", name=keep + "on")
                    transp(outT[:], outn[:])
                    return outn, outT

                z4n, z4T = conv_l3(z3n[:], z3T[:], "bigw3", bias4, AF.Tanh, "c4")
                o5n, o5T = conv_l3(z4n[:], z4T[:], "bigw4", bias5, AF.Copy, "c5")

            # ================= MLP input assembly =================
            with nc.named_scope("mlp_in"):
                nc.sync.dma_start(
                    out=x_loc.ap().rearrange("b (n c) -> n b c", c=32),
                    in_=o5n[:].rearrange("n (b c) -> n b c", c=32))
                ag(x_loc.ap(), x_all.ap())
                xT_sb = bigpool.tile([32, 4096], F16, tag="xT_sb", name="xT_sb")
                nc.sync.dma_start(out=xT_sb[:], in_=x_all[:, :])
                act6 = bigpool.tile([128, 1024], F16, tag="act6", name="act6")
                for i in range(32):
                    transp(xT_sb[:, 128 * i:128 * (i + 1)], act6[:, 32 * i:32 * i + 32])

            # ================= MLP =================
            def mlp_layer(li, act_sb, out_sb):
                g_t = load_const(f"g{li}", F32)
                be_t = load_const(f"be{li}", F32)
                wt = wlpool.tile([128, 32 * 512], F16, tag="wld", name="wld")
                nc.sync.dma_start(out=wt[:, :8192], in_=ein[f"w{li}"][:, :8192])
                nc.sync.dma_start(out=wt[:, 8192:], in_=ein[f"w{li}"][:, 8192:])
                acc = apool.tile([128, 512], F32, tag="acc", name="acc")
                for k2 in range(32):
                    nc.tensor.matmul(out=acc[:32, :], lhsT=act_sb[:, 32 * k2:32 * k2 + 32],
                                     rhs=wt[:, 512 * k2:512 * (k2 + 1)],
                                     start=(k2 == 0), stop=(k2 == 31))
                hb = wpool.tile([32, 512], F16, tag="hb", name="hb")
                nc.scalar.activation(out=hb[:], in_=acc[:32, :], func=AF.Copy)
                for c in range(4):
                    hc = wpool.tile([128, 32], F16, tag="hc", name="hc")
                    transp(hb[:, 128 * c:128 * (c + 1)], hc[:])
                    st6 = wpool.tile([128, 6], F32, tag="b_st6", name="b_st6")
                    nc.vector.bn_stats(out=st6[:], in_=hc[:])
                    mv = wpool.tile([128, 2], F32, tag="b_mv", name="b_mv")
                    nc.vector.bn_aggr(out=mv[:], in_=st6[:])
                    sd = wpool.tile([128, 1], F32, tag="b_sd", name="b_sd")
                    nc.scalar.activation(out=sd[:], in_=mv[:, 1:2], func=AF.Sqrt,
                                         bias=eps_t[:, 0:1])
                    rs = wpool.tile([128, 1], F32, tag="b_rs", name="b_rs")
                    nc.vector.reciprocal(rs[:], sd[:])
                    a_ = wpool.tile([128, 1], F32, tag="b_a", name="b_a")
                    nc.vector.tensor_mul(a_[:], rs[:], g_t[:, c:c + 1])
                    sh = wpool.tile([128, 1], F32, tag="b_sh", name="b_sh")
                    nc.vector.scalar_tensor_tensor(out=sh[:], in0=mv[:, 0:1], scalar=-1.0,
                                                   in1=a_[:], op0=ALU.mult, op1=ALU.mult)
                    nc.vector.tensor_add(sh[:], sh[:], be_t[:, c:c + 1])
                    nc.scalar.activation(out=out_sb[:, 32 * c:32 * c + 32], in_=hc[:],
                                         func=AF.Relu, scale=a_[:, 0:1], bias=sh[:, 0:1])

            with nc.named_scope("mlp6"):
                h6 = bigpool.tile([128, 128], F16, tag="h6sb", name="h6sb")
                mlp_layer(6, act6, h6)
                nc.sync.dma_start(out=h6_loc.ap(), in_=h6[:])
                ag(h6_loc.ap(), h6_all.ap())
            with nc.named_scope("mlp7"):
                act7 = bigpool.tile([128, 1024], F16, tag="act7", name="act7")
                for r in range(8):
                    nc.sync.dma_start(out=act7[:, 128 * r:128 * (r + 1)],
                                      in_=h6_all[128 * r:128 * (r + 1), :])
                h7 = bigpool.tile([128, 128], F16, tag="h7sb", name="h7sb")
                mlp_layer(7, act7, h7)
                nc.sync.dma_start(out=h7_loc.ap(), in_=h7[:])
                ag(h7_loc.ap(), h7_all.ap())
            with nc.named_scope("mlp8"):
                act8 = bigpool.tile([128, 1024], F16, tag="act8", name="act8")
                for r in range(8):
                    nc.sync.dma_start(out=act8[:, 128 * r:128 * (r + 1)],
                                      in_=h7_all[128 * r:128 * (r + 1), :])
                h8 = bigpool.tile([128, 128], F16, tag="h8sb", name="h8sb")
                mlp_layer(8, act8, h8)

            with nc.named_scope("mlp9"):
                w9t = cpool.tile([128, 512], F16, tag="w9t", name="w9t")
                nc.sync.dma_start(out=w9t[:], in_=ein["w9"][:, :])
                acc9 = apool.tile([128, 512], F32, tag="acc", name="acc9")
                for c in range(4):
                    nc.tensor.matmul(out=acc9[:32, :128], lhsT=h8[:, 32 * c:32 * c + 32],
                                     rhs=w9t[:, 128 * c:128 * (c + 1)],
                                     start=(c == 0), stop=(c == 3))
                p9sb = wpool.tile([32, 128], F32, tag="p9sb", name="p9sb")
                nc.scalar.activation(out=p9sb[:], in_=acc9[:32, :128], func=AF.Copy)
                nc.sync.dma_start(out=p9_loc.ap(), in_=p9sb[:])
                ag(p9_loc.ap(), p9_all.ap())
                tot = wpool.tile([32, 128], F32, tag="f_tot", name="f_tot")
                nc.sync.dma_start(out=tot[:], in_=p9_all[0:32, :])
                for k in range(1, 8):
                    pk = wpool.tile([32, 128], F32, tag="f_pk", name="f_pk")
                    nc.sync.dma_start(out=pk[:], in_=p9_all[32 * k:32 * (k + 1), :])
                    nc.vector.tensor_add(tot[:], tot[:], pk[:])
                totT = wpool.tile([128, 32], F32, tag="f_totT", name="f_totT")
                pst = ppool.tile([128, 512], F32, tag="ps", name="pst")
                identf = cpool.tile([32, 32], F32, tag="identf", name="identf")
                nc.scalar.activation(out=identf[:], in_=ident[:32, :32], func=AF.Copy)
                nc.tensor.transpose(out=pst[:128, :32], in_=tot[:], identity=identf[:])
                nc.scalar.activation(out=totT[:], in_=pst[:128, :32], func=AF.Copy)
                st6 = wpool.tile([128, 6], F32, tag="f_st6", name="f_st6")
                nc.vector.bn_stats(out=st6[:], in_=totT[:])
                mv = wpool.tile([128, 2], F32, tag="f_mv", name="f_mv")
                nc.vector.bn_aggr(out=mv[:], in_=st6[:])
                mu_ = mv[:, 0:1]
                sdf = wpool.tile([128, 1], F32, tag="f_sd", name="f_sd")
                nc.scalar.activation(out=sdf[:], in_=mv[:, 1:2], func=AF.Sqrt, bias=eps_t[:, 0:1])
                rs = wpool.tile([128, 1], F32, tag="f_rs", name="f_rs")
                nc.vector.reciprocal(rs[:], sdf[:])
                neg = wpool.tile([128, 1], F32, tag="f_neg", name="f_neg")
                nc.vector.scalar_tensor_tensor(out=neg[:], in0=mu_, scalar=-1.0,
                                               in1=rs[:], op0=ALU.mult, op1=ALU.mult)
                outt = wpool.tile([128, 32], F32, tag="f_out", name="f_out")
                nc.scalar.activation(out=outt[:], in_=totT[:], func=AF.Identity,
                                     scale=rs[:, 0:1], bias=neg[:, 0:1])
                nc.sync.dma_start(out=out_mu[:, :], in_=outt[:])

    nc.compile()
    return nc


# ---------------------------------------------------------------- entry point
def kernel(**inputs) -> np.ndarray:
    per_core, meta = _host_prep(inputs)
    if "prog" not in _CACHE:
        _CACHE["prog"] = _build_nc(meta, per_core[0])
    nc = _CACHE["prog"]
    res = bass_utils.run_bass_kernel_spmd(nc, per_core, core_ids=list(range(NCORES)))
    return np.ascontiguousarray(res.results[0]["mu"].T)
